# revision 1
# baseline (speedup 1.0000x reference)
"""Trainium2 Bass kernel for nn_ConservativeDynamicCurvatureMLP.

Data-parallel over 8 NeuronCores: the batch (8192) is sharded into 8
local shards of 1024 rows; all weights are replicated. The curvature
scalar c_avg couples the shards through a global mean, handled with a
single-scalar AllReduce.

Math (reference):
    h = tanh(x @ W1 + b1)
    u = sigmoid(h @ W2 + b2)
    c = clip(mean(MIN_C + (MAX_C-MIN_C) * sigmoid(relu(x@cp_w1.T+cp_b1)@cp_w2.T+cp_b2)), MIN_C, MAX_C)
    z = poincare_ball_layer(h, u, c, T)
    out = z @ Wo + bo

The poincare layer collapses algebraically to z = alpha(row)*h + beta(row)*u
where alpha/beta are scalar functions of the row statistics
x2=||h||^2, y2=||u||^2, xy=<h,u> and c (verified to ~1e-6 against the
reference).  The NaN fallback (z <- h if any(isnan(z))) can only trigger when
den = 1 + 2c<x,y> + c^2 x2 y2 == -EPS exactly (measure-zero); it is omitted.

On-device layout is feature-major throughout: activations live as
[128 feature-partitions, kt, 1024 batch-cols] so every matmul consumes the
previous one's output directly (weights are the stationary operand in natural
[K, M] layout) and no transposes are needed.  Row statistics are computed with
a ones-vector matmul (partition reduction on the PE); the per-row scalar chain
runs batch-major on [128, 8] tiles via a small DRAM bounce.
"""

import tempfile
from contextlib import ExitStack

import numpy as np
import ml_dtypes

import concourse.bass as bass
import concourse.bacc as bacc
import concourse.mybir as mybir
import concourse.tile as tile
from concourse.bass_utils import run_bass_kernel_spmd

P = 128
N_CORES = 8
B_FULL = 8192
BL = B_FULL // N_CORES          # 1024 rows per core
IN = 3072
HID = 4096
OUT = 1000
KI = IN // P                    # 24
KH = HID // P                   # 32
NB = BL // P                    # 8 batch tiles
MIN_C = 0.001 * 0.5
MAX_C = 0.001 * 2.0
T_CONST = 0.7
EPS = 1e-7

dt = mybir.dt
AF = mybir.ActivationFunctionType
ALU = mybir.AluOpType
BF = ml_dtypes.bfloat16

_nc_cache = []


def _build(with_b1, with_b2):
    nc = bacc.Bacc("TRN2", target_bir_lowering=False, debug=False,
                   num_devices=N_CORES)

    xT_d = nc.dram_tensor("xT", [KI, P, BL], dt.bfloat16, kind="ExternalInput")
    # weight rows: w1r[mh, p, ki, q] = W1[ki*128+p, mh*128+q] -> one contiguous
    # 768KB DMA per output row-tile instead of 24 strided tile DMAs
    w1_d = nc.dram_tensor("w1", [KH, P, KI, P], dt.bfloat16, kind="ExternalInput")
    w2_d = nc.dram_tensor("w2", [KH, P, KH, P], dt.bfloat16, kind="ExternalInput")
    wo_d = nc.dram_tensor("wo", [KH, P, OUT], dt.bfloat16, kind="ExternalInput")
    cpw1_d = nc.dram_tensor("cpw1", [KI, P, 16], dt.bfloat16, kind="ExternalInput")
    cpw2_d = nc.dram_tensor("cpw2", [16, 1], dt.bfloat16, kind="ExternalInput")
    cpb1_d = nc.dram_tensor("cpb1", [16, 1], dt.float32, kind="ExternalInput")
    cpb2_d = nc.dram_tensor("cpb2", [1, 1], dt.float32, kind="ExternalInput")
    b1_d = nc.dram_tensor("b1", [P, KH], dt.float32, kind="ExternalInput") if with_b1 else None
    b2_d = nc.dram_tensor("b2", [P, KH], dt.float32, kind="ExternalInput") if with_b2 else None
    out_d = nc.dram_tensor("out", [BL, OUT], dt.float32, kind="ExternalOutput")

    f32 = dt.float32
    bf16 = dt.bfloat16

    with tile.TileContext(nc) as tc, ExitStack() as ctx:
        const = ctx.enter_context(tc.tile_pool(name="const", bufs=1))
        big = ctx.enter_context(tc.tile_pool(name="big", bufs=1))
        htp = ctx.enter_context(tc.tile_pool(name="htp", bufs=1))
        wp = ctx.enter_context(tc.tile_pool(name="wp", bufs=2))
        wop = ctx.enter_context(tc.tile_pool(name="wop", bufs=3))
        scr = ctx.enter_context(tc.tile_pool(name="scr", bufs=4))
        sacc = ctx.enter_context(tc.tile_pool(name="sacc", bufs=1))
        abp = ctx.enter_context(tc.tile_pool(name="abp", bufs=1))
        scal = ctx.enter_context(tc.tile_pool(name="scal", bufs=1))
        outp = ctx.enter_context(tc.tile_pool(name="outp", bufs=2))
        cpp = ctx.enter_context(tc.tile_pool(name="cpp", bufs=1))
        dram = ctx.enter_context(tc.tile_pool(name="dram", bufs=1, space="DRAM"))

        V = nc.vector
        S = nc.scalar

        def sc(name, shape=(P, 8), dtype=f32):
            return scal.tile(list(shape), dtype, name=name, tag=name)

        # ---------- persistent activations (feature-major) ----------
        # xT lands in 6 batched DMAs so MM1 can start consuming ki-group 0
        # while later groups stream; weight-row DMAs are emitted inside the
        # mh loops and interleave on the same queue.
        ones = const.tile([P, 1], f32, name="ones")
        nc.vector.memset(ones, 1.0)
        xT_sb = big.tile([P, KI, BL], bf16, name="xT_sb", tag="big",
                         padded_shape=[P, KH, BL])
        # first weight row issues on the sync queue before anything else;
        # xT streams via the otherwise-idle gpsimd queue in parallel
        w1row0 = wp.tile([P, KI, P], bf16, name="w1row", tag="w")
        nc.sync.dma_start(out=w1row0, in_=w1_d[0])
        for a, b in ((0, 2), (2, 4), (4, 8), (8, 12), (12, 16), (16, 20),
                     (20, 24)):
            nc.gpsimd.dma_start(
                out=xT_sb[:, a:b, :],
                in_=xT_d[a:b].rearrange("k p b -> p k b"))
        hT_sb = htp.tile([P, KH, BL], bf16, name="hT_sb")
        if with_b1:
            b1_sb = const.tile([P, KH], f32, name="b1_sb")
            nc.sync.dma_start(out=b1_sb, in_=b1_d[:, :])
        if with_b2:
            b2_sb = const.tile([P, KH], f32, name="b2_sb")
            nc.sync.dma_start(out=b2_sb, in_=b2_d[:, :])

        with ExitStack() as ph1:
            mm = ph1.enter_context(tc.tile_pool(name="mm", bufs=3, space="PSUM"))
            stp = ph1.enter_context(tc.tile_pool(name="stp", bufs=1, space="PSUM"))
            # stat rows live at partitions 0/32/64 (PSUM write base-partition
            # constraint): x2 @ 0, y2 @ 32, xy @ 64
            stat_ps = stp.tile([P, BL], f32, name="stat_ps")

            # ---------- MM1: hT = tanh(W1.T @ xT) , x2 stats ----------
            # Row statistics: the elementwise squares/products run on ACT/DVE
            # and accumulate (fp32, DVE) across mh; the PE only does one final
            # ones-matmul partition-reduction per stat per 512-chunk.
            x2a = sacc.tile([P, BL], f32, name="x2a")
            y2a = sacc.tile([P, BL], f32, name="y2a")
            xya = sacc.tile([P, BL], f32, name="xya")
            with nc.named_scope("mm1"):
                for mh in range(KH):
                    ps = mm.tile([P, BL], f32, name="ps", tag="mm")
                    if mh == 0:
                        w1row = w1row0
                    else:
                        w1row = wp.tile([P, KI, P], bf16, name="w1row",
                                        tag="w")
                        nc.sync.dma_start(out=w1row, in_=w1_d[mh])
                    for ki in range(KI):
                        nc.tensor.matmul(ps[:, 0:512], lhsT=w1row[:, ki, :],
                                         rhs=xT_sb[:, ki, 0:512],
                                         start=(ki == 0), stop=(ki == KI - 1))
                        nc.tensor.matmul(ps[:, 512:BL], lhsT=w1row[:, ki, :],
                                         rhs=xT_sb[:, ki, 512:BL],
                                         start=(ki == 0), stop=(ki == KI - 1))
                    if with_b1:
                        S.activation(hT_sb[:, mh, :], ps, AF.Tanh,
                                     bias=b1_sb[:, mh:mh + 1])
                    else:
                        S.activation(hT_sb[:, mh, :], ps, AF.Tanh)
                    hh = scr.tile([P, BL], bf16, name="hh", tag="hh")
                    S.activation(hh, hT_sb[:, mh, :], AF.Square)
                    if mh == 0:
                        V.tensor_copy(x2a, hh)
                    else:
                        V.tensor_add(x2a, x2a, hh)
                for ch in range(2):
                    sl = slice(ch * 512, (ch + 1) * 512)
                    nc.tensor.matmul(stat_ps[0:1, sl], lhsT=ones,
                                     rhs=x2a[:, sl], start=True, stop=True,
                                     skip_group_check=True)

            # ---------- curvature predictor (after MM1: xT still resident,
            # c is only needed after MM2, so the AllReduce hides easily) ----
            with nc.named_scope("cp"):
                cpw1_sb = const.tile([P, KI, 16], bf16, name="cpw1_sb")
                nc.sync.dma_start(out=cpw1_sb,
                                  in_=cpw1_d.rearrange("k p q -> p k q"))
                cpw2_sb = const.tile([16, 1], bf16, name="cpw2_sb")
                nc.sync.dma_start(out=cpw2_sb, in_=cpw2_d[:, :])
                cpb1_sb = const.tile([16, 1], f32, name="cpb1_sb")
                nc.sync.dma_start(out=cpb1_sb, in_=cpb1_d[:, :])
                cpb2_sb = const.tile([1, 1], f32, name="cpb2_sb")
                nc.sync.dma_start(out=cpb2_sb, in_=cpb2_d[:, :])
                cph_sb = cpp.tile([16, BL], bf16, name="cph_sb")
                for ch in range(2):
                    cps = mm.tile([16, 512], f32, name="cps", tag="mm")
                    for ki in range(KI):
                        nc.tensor.matmul(
                            cps, lhsT=cpw1_sb[:, ki, :],
                            rhs=xT_sb[:, ki, ch * 512:(ch + 1) * 512],
                            start=(ki == 0), stop=(ki == KI - 1))
                    S.activation(cph_sb[:, ch * 512:(ch + 1) * 512], cps,
                                 AF.Relu, bias=cpb1_sb)
                sparts = []
                for ch in range(2):
                    c2p = mm.tile([1, 512], f32, name="c2p", tag="mm")
                    nc.tensor.matmul(c2p, lhsT=cpw2_sb,
                                     rhs=cph_sb[:16, ch * 512:(ch + 1) * 512],
                                     start=True, stop=True)
                    cpw = cpp.tile([1, 512], f32, name="cpw", tag="cpw")
                    spart = cpp.tile([1, 1], f32, name=f"spart{ch}",
                                     tag=f"spart{ch}")
                    S.activation(cpw, c2p, AF.Sigmoid, bias=cpb2_sb,
                                 accum_out=spart)
                    sparts.append(spart)
                s_loc = cpp.tile([1, 1], f32, name="s_loc")
                V.tensor_add(s_loc, sparts[0], sparts[1])
                cin = dram.tile([1, 1], f32, name="cin")
                cout = dram.tile([1, 1], f32, name="cout")
                nc.sync.dma_start(out=cin, in_=s_loc)
                nc.gpsimd.collective_compute(
                    "AllReduce", ALU.add,
                    replica_groups=[list(range(N_CORES))],
                    ins=[cin.opt()], outs=[cout.opt()])
                s_b = sc("s_b", (P, 1))
                nc.gpsimd.dma_start(out=s_b, in_=cout.to_broadcast([P, 1]))
                # c = clip(MIN_C + (MAX_C-MIN_C)*mean(c_pred))
                c_b = sc("c_b", (P, 1))
                V.tensor_scalar(out=c_b, in0=s_b,
                                scalar1=(MAX_C - MIN_C) / B_FULL,
                                scalar2=MIN_C, op0=ALU.mult, op1=ALU.add)
                V.tensor_scalar_min(out=c_b, in0=c_b, scalar1=MAX_C)
                V.tensor_scalar_max(out=c_b, in0=c_b, scalar1=MIN_C)
                negc_b = sc("negc_b", (P, 1))
                V.tensor_scalar_mul(out=negc_b, in0=c_b, scalar1=-1.0)
                twoc_b = sc("twoc_b", (P, 1))
                V.tensor_scalar_mul(out=twoc_b, in0=c_b, scalar1=2.0)
                neg2c_b = sc("neg2c_b", (P, 1))
                V.tensor_scalar_mul(out=neg2c_b, in0=c_b, scalar1=-2.0)
                c2_b = sc("c2_b", (P, 1))
                V.tensor_mul(c2_b, c_b, c_b)

            # ---------- MM2: uT = sigmoid(W2.T @ hT) , y2/xy stats ----------
            uT_sb = big.tile([P, KH, BL], bf16, name="uT_sb", tag="big")
            with nc.named_scope("mm2"):
                for mh in range(KH):
                    ps = mm.tile([P, BL], f32, name="ps", tag="mm")
                    w2row = wp.tile([P, KH, P], bf16, name="w2row", tag="w")
                    nc.sync.dma_start(out=w2row, in_=w2_d[mh])
                    for kh in range(KH):
                        nc.tensor.matmul(ps[:, 0:512], lhsT=w2row[:, kh, :],
                                         rhs=hT_sb[:, kh, 0:512],
                                         start=(kh == 0), stop=(kh == KH - 1))
                        nc.tensor.matmul(ps[:, 512:BL], lhsT=w2row[:, kh, :],
                                         rhs=hT_sb[:, kh, 512:BL],
                                         start=(kh == 0), stop=(kh == KH - 1))
                    if with_b2:
                        S.activation(uT_sb[:, mh, :], ps, AF.Sigmoid,
                                     bias=b2_sb[:, mh:mh + 1])
                    else:
                        S.activation(uT_sb[:, mh, :], ps, AF.Sigmoid)
                    uu = scr.tile([P, BL], bf16, name="uu", tag="hh")
                    S.activation(uu, uT_sb[:, mh, :], AF.Square)
                    hu = scr.tile([P, BL], bf16, name="hu", tag="hh")
                    V.tensor_mul(hu, hT_sb[:, mh, :], uT_sb[:, mh, :])
                    if mh == 0:
                        V.tensor_copy(y2a, uu)
                        V.tensor_copy(xya, hu)
                    else:
                        V.tensor_add(y2a, y2a, uu)
                        V.tensor_add(xya, xya, hu)
                for ch in range(2):
                    sl = slice(ch * 512, (ch + 1) * 512)
                    nc.tensor.matmul(stat_ps[32:33, sl], lhsT=ones,
                                     rhs=y2a[:, sl], start=True, stop=True,
                                     skip_group_check=True)
                    nc.tensor.matmul(stat_ps[64:65, sl], lhsT=ones,
                                     rhs=xya[:, sl], start=True, stop=True,
                                     skip_group_check=True)

            # ---------- stats -> batch-major, per column-half ----------
            # split by 512-column half so the half-0 scalar chain (which
            # gates zcomb0/mmo0) starts without waiting for half-1 plumbing
            with nc.named_scope("stats"):
                stats_sb = scal.tile([P, BL], f32, name="stats_sb", tag="stats_sb")
                st_d = dram.tile([3, BL], f32, name="st_d")
                for ch in range(2):
                    hsl = slice(ch * 512, (ch + 1) * 512)
                    for i, r in enumerate((0, 32, 64)):
                        S.copy(stats_sb[r:r + 1, hsl], stat_ps[r:r + 1, hsl])
                        nc.sync.dma_start(out=st_d[i, hsl],
                                          in_=stats_sb[r:r + 1, hsl])

        # psum pools (mm, stp) released here

        # ---------- per-row scalar chain (batch-major [128, 4] per half) ---
        alpha_b = abp.tile([P, BL], f32, name="alpha_b")
        beta_b = abp.tile([P, BL], f32, name="beta_b")
        ab_d = dram.tile([2, BL], f32, name="ab_d")

        def scalar_chain(ch):
            hsl = slice(ch * 512, (ch + 1) * 512)

            def sch(name):
                return sc(f"{name}_{ch}", (P, 4))

            x2 = sch("x2")
            y2 = sch("y2")
            xy = sch("xy")
            for i, t in enumerate((x2, y2, xy)):
                nc.sync.dma_start(
                    out=t, in_=st_d[i, hsl].rearrange("(j p) -> p j", p=P))
            w = sch("w")
            V.scalar_tensor_tensor(out=w, in0=xy, scalar=-2.0, in1=y2,
                                   op0=ALU.mult, op1=ALU.add)
            A1 = sch("A1")
            V.tensor_scalar(out=A1, in0=w, scalar1=c_b, scalar2=1.0,
                            op0=ALU.mult, op1=ALU.add)
            A2 = sch("A2")
            V.tensor_scalar(out=A2, in0=x2, scalar1=negc_b, scalar2=1.0,
                            op0=ALU.mult, op1=ALU.add)
            p1 = sch("p1")
            V.tensor_mul(p1, x2, y2)
            den = sch("den")
            V.tensor_scalar(out=den, in0=p1, scalar1=c2_b, scalar2=1.0,
                            op0=ALU.mult, op1=ALU.add)
            V.scalar_tensor_tensor(out=den, in0=xy, scalar=neg2c_b, in1=den,
                                   op0=ALU.mult, op1=ALU.add)
            V.tensor_scalar_add(out=den, in0=den, scalar1=EPS)
            D = sch("D")
            V.reciprocal(D, den)
            # ||a||^2 = D^2 (A1^2 x2 - 2 A1 A2 xy + A2^2 y2)
            t1 = sch("t1")
            V.tensor_mul(t1, A1, A1)
            V.tensor_mul(t1, t1, x2)
            t2 = sch("t2")
            V.tensor_mul(t2, A1, A2)
            V.tensor_mul(t2, t2, xy)
            t3 = sch("t3")
            V.tensor_mul(t3, A2, A2)
            V.tensor_mul(t3, t3, y2)
            na2 = sch("na2")
            V.scalar_tensor_tensor(out=na2, in0=t2, scalar=-2.0, in1=t1,
                                   op0=ALU.mult, op1=ALU.add)
            V.tensor_add(na2, na2, t3)
            dsq = sch("dsq")
            V.tensor_mul(dsq, D, D)
            V.tensor_mul(na2, na2, dsq)
            # q = sqrt(c * na2) with one Newton step (ACT sqrt is low precision)
            q2 = sch("q2")
            V.tensor_scalar(out=q2, in0=na2, scalar1=c_b, scalar2=None,
                            op0=ALU.mult)
            q0 = sch("q0")
            S.activation(q0, q2, AF.Sqrt)
            V.tensor_scalar_max(out=q0, in0=q0, scalar1=1e-20)
            r0 = sch("r0")
            V.reciprocal(r0, q0)
            q = sch("q")
            V.tensor_mul(q, q2, r0)
            V.tensor_add(q, q, q0)
            V.tensor_scalar_mul(out=q, in0=q, scalar1=0.5)
            arg = sch("arg")
            V.tensor_scalar_min(out=arg, in0=q, scalar1=1.0 - 1e-5)
            # artanh(arg) = 0.5 ln((1+arg)/(1-arg)); t = tanh(T*artanh)/q
            opp = sch("opp")
            V.tensor_scalar(out=opp, in0=arg, scalar1=-1.0, scalar2=1.0,
                            op0=ALU.mult, op1=ALU.add)
            opn = sch("opn")
            V.tensor_scalar_add(out=opn, in0=arg, scalar1=1.0)
            rr = sch("rr")
            V.reciprocal(rr, opp)
            rat = sch("rat")
            V.tensor_mul(rat, opn, rr)
            lg = sch("lg")
            S.activation(lg, rat, AF.Ln)
            th = sch("th")
            S.activation(th, lg, AF.Tanh, scale=T_CONST * 0.5)
            rq = sch("rq")
            V.reciprocal(rq, q)
            tm = sch("tm")
            V.tensor_mul(tm, th, rq)
            # <h,a> = D (A2 xy - A1 x2)
            s1_ = sch("s1_")
            V.tensor_mul(s1_, A1, x2)
            s2_ = sch("s2_")
            V.tensor_mul(s2_, A2, xy)
            ha = sch("ha")
            V.tensor_sub(ha, s2_, s1_)
            V.tensor_mul(ha, ha, D)
            hm = sch("hm")
            V.tensor_mul(hm, tm, ha)
            tsq = sch("tsq")
            V.tensor_mul(tsq, tm, tm)
            m2 = sch("m2")
            V.tensor_mul(m2, tsq, na2)
            w2s = sch("w2s")
            V.scalar_tensor_tensor(out=w2s, in0=hm, scalar=2.0, in1=m2,
                                   op0=ALU.mult, op1=ALU.add)
            B1 = sch("B1")
            V.tensor_scalar(out=B1, in0=w2s, scalar1=c_b, scalar2=1.0,
                            op0=ALU.mult, op1=ALU.add)
            p2 = sch("p2")
            V.tensor_mul(p2, x2, m2)
            den2 = sch("den2")
            V.tensor_scalar(out=den2, in0=p2, scalar1=c2_b, scalar2=1.0,
                            op0=ALU.mult, op1=ALU.add)
            V.scalar_tensor_tensor(out=den2, in0=hm, scalar=twoc_b, in1=den2,
                                   op0=ALU.mult, op1=ALU.add)
            V.tensor_scalar_add(out=den2, in0=den2, scalar1=EPS)
            D2 = sch("D2")
            V.reciprocal(D2, den2)
            g = sch("g")
            V.tensor_mul(g, A2, tm)
            V.tensor_mul(g, g, D)
            w3 = sch("w3")
            V.tensor_mul(w3, g, A1)
            V.tensor_sub(w3, B1, w3)
            alpha_bm = sch("alpha_bm")
            V.tensor_mul(alpha_bm, w3, D2)
            w4 = sch("w4")
            V.tensor_mul(w4, g, A2)
            beta_bm = sch("beta_bm")
            V.tensor_mul(beta_bm, w4, D2)
            # bounce to DRAM in batch-linear order, broadcast back
            nc.sync.dma_start(
                out=ab_d[0, hsl].rearrange("(j p) -> p j", p=P), in_=alpha_bm)
            nc.sync.dma_start(
                out=ab_d[1, hsl].rearrange("(j p) -> p j", p=P), in_=beta_bm)
            nc.gpsimd.dma_start(out=alpha_b[:, hsl],
                                in_=ab_d[0:1, hsl].to_broadcast([P, 512]))
            nc.gpsimd.dma_start(out=beta_b[:, hsl],
                                in_=ab_d[1:2, hsl].to_broadcast([P, 512]))

        with nc.named_scope("scalars"):
            scalar_chain(0)
            scalar_chain(1)

        # ---------- z = alpha*h + beta*u (overwrites uT in place),
        # then out = z @ Wo.  Processed in two batch-column halves so the
        # MMo matmuls of half 0 overlap the DVE z-combine of half 1.
        with ExitStack() as ph2:
            mmo = ph2.enter_context(tc.tile_pool(name="mmo", bufs=8,
                                                 space="PSUM"))
            for bg in range(2):
                csl = slice(bg * 512, (bg + 1) * 512)
                with nc.named_scope(f"zcomb{bg}"):
                    for kh in range(KH):
                        t1z = scr.tile([P, 512], bf16, name="t1z", tag="zz",
                                       bufs=4)
                        V.tensor_mul(t1z, hT_sb[:, kh, csl], alpha_b[:, csl])
                        t2z = scr.tile([P, 512], bf16, name="t2z", tag="zz",
                                       bufs=4)
                        V.tensor_mul(t2z, uT_sb[:, kh, csl], beta_b[:, csl])
                        V.tensor_add(uT_sb[:, kh, csl], t1z, t2z)
                with nc.named_scope(f"mmo{bg}"):
                    pso = [mmo.tile([P, 500], f32, name=f"pso{bg}_{i}",
                                    tag="mmo") for i in range(8)]
                    for kh in range(KH):
                        wot = wop.tile([P, OUT], bf16, name="wot", tag="wo")
                        nc.sync.dma_start(out=wot, in_=wo_d[kh])
                        for i in range(4):
                            b = bg * 4 + i
                            for och in range(2):
                                nc.tensor.matmul(
                                    pso[i * 2 + och],
                                    lhsT=uT_sb[:, kh, b * P:(b + 1) * P],
                                    rhs=wot[:, och * 500:(och + 1) * 500],
                                    start=(kh == 0), stop=(kh == KH - 1))
                    for i in range(4):
                        b = bg * 4 + i
                        ob = outp.tile([P, OUT], f32, name="ob", tag="ob")
                        S.copy(ob[:, 0:500], pso[i * 2])
                        V.tensor_copy(ob[:, 500:OUT], pso[i * 2 + 1])
                        nc.sync.dma_start(out=out_d[b * P:(b + 1) * P, :],
                                          in_=ob)

    nc.compile()
    return nc


def _get_nc(with_b1, with_b2):
    for k, v in _nc_cache:
        if k == (with_b1, with_b2):
            return v
    nc = _build(with_b1, with_b2)
    _nc_cache.append(((with_b1, with_b2), nc))
    return nc


def kernel(x, W1, b1, W2, b2, Wo, bo, cp_w1, cp_b1, cp_w2, cp_b2,
           _trace=False, _tmpdir=None):
    x = np.asarray(x, dtype=np.float32)
    with_b1 = bool(np.any(b1))
    with_b2 = bool(np.any(b2))
    nc = _get_nc(with_b1, with_b2)

    # w1r[mh, p, ki, q] = W1[ki*128+p, mh*128+q]
    w1_t = np.ascontiguousarray(
        np.asarray(W1, np.float32).reshape(KI, P, KH, P).transpose(2, 1, 0, 3)
    ).astype(BF)
    w2_t = np.ascontiguousarray(
        np.asarray(W2, np.float32).reshape(KH, P, KH, P).transpose(2, 1, 0, 3)
    ).astype(BF)
    wo_t = np.asarray(Wo, np.float32).reshape(KH, P, OUT).astype(BF)
    cpw1_t = np.ascontiguousarray(
        np.asarray(cp_w1, np.float32).T.reshape(KI, P, 16)).astype(BF)
    cpw2_t = np.asarray(cp_w2, np.float32).reshape(1, 16).T.astype(BF)
    cpw2_t = np.ascontiguousarray(cpw2_t)
    cpb1_t = np.asarray(cp_b1, np.float32).reshape(16, 1)
    cpb2_t = np.asarray(cp_b2, np.float32).reshape(1, 1)
    b1_t = np.ascontiguousarray(np.asarray(b1, np.float32).reshape(KH, P).T)
    b2_t = np.ascontiguousarray(np.asarray(b2, np.float32).reshape(KH, P).T)

    in_maps = []
    for c in range(N_CORES):
        shard = x[c * BL:(c + 1) * BL]
        xT = np.ascontiguousarray(shard.T).reshape(KI, P, BL).astype(BF)
        m = {"xT": xT, "w1": w1_t, "w2": w2_t, "wo": wo_t,
             "cpw1": cpw1_t, "cpw2": cpw2_t, "cpb1": cpb1_t, "cpb2": cpb2_t}
        if with_b1:
            m["b1"] = b1_t
        if with_b2:
            m["b2"] = b2_t
        in_maps.append(m)

    kw = {}
    if _trace:
        kw = dict(trace=True, tmpdir=_tmpdir or tempfile.mkdtemp(prefix="cdk_"))
    res = run_bass_kernel_spmd(nc, in_maps, list(range(N_CORES)), **kw)

    out = np.concatenate([res.results[c]["out"] for c in range(N_CORES)], axis=0)
    bo = np.asarray(bo, np.float32)
    if np.any(bo):
        out = out + bo
    if _trace:
        kernel._last_result = res
    return out



# revision 2
# speedup vs baseline: 1.2105x; 1.2105x over previous
"""Trainium2 Bass kernel for nn_ConservativeDynamicCurvatureMLP.

Data-parallel over 8 NeuronCores: the batch (8192) is sharded into 8
local shards of 1024 rows; all weights are replicated. The curvature
scalar c_avg couples the shards through a global mean, handled with a
single-scalar AllReduce.

Math (reference):
    h = tanh(x @ W1 + b1)
    u = sigmoid(h @ W2 + b2)
    c = clip(mean(MIN_C + (MAX_C-MIN_C) * sigmoid(relu(x@cp_w1.T+cp_b1)@cp_w2.T+cp_b2)), MIN_C, MAX_C)
    z = poincare_ball_layer(h, u, c, T)
    out = z @ Wo + bo

The poincare layer collapses algebraically to z = alpha(row)*h + beta(row)*u
where alpha/beta are scalar functions of the row statistics
x2=||h||^2, y2=||u||^2, xy=<h,u> and c.  The NaN fallback is measure-zero
and omitted.

Perf notes (measured on hw):
  - With 8 cores active the PE is power-throttled to ~2.0 GHz (HAM state
    31), so bf16 N=512 matmuls pace at ~263 ns.  The only lever is fewer
    PE cycles: MM2 (h@W2, the largest matmul) runs as fp8-e4m3 DoubleRow
    (256-deep contraction per pass, 2x FLOP rate, measured 216 ns/MM at
    2.4 GHz in isolation).  W2 is host-converted to e4m3 * 1024 (clipped
    to +-240, the TRN e4m3 max); the 1/1024 folds into the sigmoid's
    activation scale.  h is cast bf16->fp8 per row-tile on the DVE.
    MM1 / MMo stay bf16: fp8 there pushes rel-err too close to the 2e-2
    gate (simulated 1.96e-2 vs 1.26e-2 for MM2-only).
  - Row statistics go batch-major directly via tiny stationary-squares
    matmuls (lhsT = accumulated squares block, rhs = ones) instead of a
    feature-major ones-reduction + DRAM bounce, shortening the serial
    stats -> alpha/beta tail.  The per-row scalar chain runs once on
    [128, 8] tiles reading stats straight from SBUF.
"""

import tempfile
from contextlib import ExitStack

import numpy as np
import ml_dtypes

import concourse.bass as bass
import concourse.bacc as bacc
import concourse.mybir as mybir
import concourse.tile as tile
from concourse.bass_utils import run_bass_kernel_spmd

P = 128
N_CORES = 8
B_FULL = 8192
BL = B_FULL // N_CORES          # 1024 rows per core
IN = 3072
HID = 4096
OUT = 1000
KI = IN // P                    # 24
KH = HID // P                   # 32
NB = BL // P                    # 8 batch tiles
MIN_C = 0.001 * 0.5
MAX_C = 0.001 * 2.0
T_CONST = 0.7
EPS = 1e-7
W2S = 1024.0                    # host-side fp8 scale for W2
E4MAX = 240.0                   # TRN e4m3 saturation

dt = mybir.dt
AF = mybir.ActivationFunctionType
ALU = mybir.AluOpType
PM = mybir.MatmulPerfMode
BF = ml_dtypes.bfloat16
E4 = ml_dtypes.float8_e4m3

_nc_cache = []


def _build(with_b1, with_b2):
    nc = bacc.Bacc("TRN2", target_bir_lowering=False, debug=False,
                   num_devices=N_CORES)

    xT_d = nc.dram_tensor("xT", [KI, P, BL], dt.bfloat16, kind="ExternalInput")
    # W1 rows split in two halves for finer DMA/SBUF: w1h[mh, g, p, kk, q] =
    # W1[(g*12+kk)*128+p, mh*128+q]
    w1_d = nc.dram_tensor("w1", [KH, 2, P, KI // 2, P], dt.bfloat16,
                          kind="ExternalInput")
    # W2 in fp8 (x1024): w2r[mh, p, kh, q] = W2[kh*128+p, mh*128+q]
    w2_d = nc.dram_tensor("w2", [KH, P, KH, P], dt.float8e4, kind="ExternalInput")
    wo_d = nc.dram_tensor("wo", [KH, P, OUT], dt.bfloat16, kind="ExternalInput")
    cpw1_d = nc.dram_tensor("cpw1", [KI, P, 16], dt.bfloat16, kind="ExternalInput")
    cpw2_d = nc.dram_tensor("cpw2", [16, 1], dt.bfloat16, kind="ExternalInput")
    cpb1_d = nc.dram_tensor("cpb1", [16, 1], dt.float32, kind="ExternalInput")
    cpb2_d = nc.dram_tensor("cpb2", [1, 1], dt.float32, kind="ExternalInput")
    b1_d = nc.dram_tensor("b1", [P, KH], dt.float32, kind="ExternalInput") if with_b1 else None
    b2_d = nc.dram_tensor("b2", [P, KH], dt.float32, kind="ExternalInput") if with_b2 else None
    out_d = nc.dram_tensor("out", [BL, OUT], dt.float32, kind="ExternalOutput")

    f32 = dt.float32
    bf16 = dt.bfloat16
    fp8 = dt.float8e4

    with tile.TileContext(nc) as tc, ExitStack() as ctx:
        const = ctx.enter_context(tc.tile_pool(name="const", bufs=1))
        big = ctx.enter_context(tc.tile_pool(name="big", bufs=1))
        htp = ctx.enter_context(tc.tile_pool(name="htp", bufs=1))
        hqp = ctx.enter_context(tc.tile_pool(name="hqp", bufs=1))
        wp = ctx.enter_context(tc.tile_pool(name="wp", bufs=2))
        wop = ctx.enter_context(tc.tile_pool(name="wop", bufs=2))
        scr = ctx.enter_context(tc.tile_pool(name="scr", bufs=2))
        zzp = ctx.enter_context(tc.tile_pool(name="zzp", bufs=4))
        sacc = ctx.enter_context(tc.tile_pool(name="sacc", bufs=1))
        abp = ctx.enter_context(tc.tile_pool(name="abp", bufs=1))
        scal = ctx.enter_context(tc.tile_pool(name="scal", bufs=1))
        outp = ctx.enter_context(tc.tile_pool(name="outp", bufs=2))
        cpp = ctx.enter_context(tc.tile_pool(name="cpp", bufs=1))
        dram = ctx.enter_context(tc.tile_pool(name="dram", bufs=1, space="DRAM"))

        V = nc.vector
        S = nc.scalar

        def sc(name, shape=(P, 8), dtype=f32):
            return scal.tile(list(shape), dtype, name=name, tag=name)

        # ---------- persistent activations (feature-major) ----------
        onesb = const.tile([P, 1], bf16, name="onesb")
        nc.vector.memset(onesb, 1.0)
        xT_sb = big.tile([P, KI, BL], bf16, name="xT_sb", tag="big",
                         padded_shape=[P, KH, BL])
        # first weight half-row issues on the sync queue before anything else;
        # xT streams via the otherwise-idle gpsimd queue in parallel
        w1row0 = wp.tile([P, KI // 2, P], bf16, name="w1row", tag="w1")
        nc.sync.dma_start(out=w1row0, in_=w1_d[0, 0])
        for a, b in ((0, 2), (2, 4), (4, 8), (8, 12), (12, 16), (16, 20),
                     (20, 24)):
            nc.gpsimd.dma_start(
                out=xT_sb[:, a:b, :],
                in_=xT_d[a:b].rearrange("k p b -> p k b"))
        hT_sb = htp.tile([P, KH, BL], bf16, name="hT_sb")
        hTq_sb = hqp.tile([P, KH, BL], fp8, name="hTq_sb")
        if with_b1:
            b1_sb = const.tile([P, KH], f32, name="b1_sb")
            nc.sync.dma_start(out=b1_sb, in_=b1_d[:, :])
        if with_b2:
            b2_sb = const.tile([P, KH], f32, name="b2_sb")
            nc.sync.dma_start(out=b2_sb, in_=b2_d[:, :])

        with ExitStack() as ph1:
            mm = ph1.enter_context(tc.tile_pool(name="mm", bufs=3, space="PSUM"))
            stp = ph1.enter_context(tc.tile_pool(name="stp", bufs=1, space="PSUM"))
            # batch-major stats land here: cols 0:8 x2, 8:16 y2, 16:24 xy
            stat_ps = stp.tile([P, 24], f32, name="stat_ps")

            # ---------- MM1: hT = tanh(W1.T @ xT) (bf16) ----------
            x2a = sacc.tile([P, BL], bf16, name="x2a")
            y2a = sacc.tile([P, BL], bf16, name="y2a")
            xya = sacc.tile([P, BL], bf16, name="xya")
            with nc.named_scope("mm1"):
                for mh in range(KH):
                    ps = mm.tile([P, BL], f32, name="ps", tag="mm")
                    if mh == 0:
                        w1a = w1row0
                    else:
                        w1a = wp.tile([P, KI // 2, P], bf16, name="w1row",
                                      tag="w1")
                        nc.sync.dma_start(out=w1a, in_=w1_d[mh, 0])
                    w1b = wp.tile([P, KI // 2, P], bf16, name="w1row", tag="w1")
                    nc.sync.dma_start(out=w1b, in_=w1_d[mh, 1])
                    for ki in range(KI):
                        wrow = w1a if ki < 12 else w1b
                        kk = ki % 12
                        nc.tensor.matmul(ps[:, 0:512], lhsT=wrow[:, kk, :],
                                         rhs=xT_sb[:, ki, 0:512],
                                         start=(ki == 0), stop=(ki == KI - 1))
                        nc.tensor.matmul(ps[:, 512:BL], lhsT=wrow[:, kk, :],
                                         rhs=xT_sb[:, ki, 512:BL],
                                         start=(ki == 0), stop=(ki == KI - 1))
                    if with_b1:
                        S.activation(hT_sb[:, mh, :], ps, AF.Tanh,
                                     bias=b1_sb[:, mh:mh + 1])
                    else:
                        S.activation(hT_sb[:, mh, :], ps, AF.Tanh)
                    # fp8 copy for MM2's DoubleRow rhs
                    V.tensor_copy(hTq_sb[:, mh, :], hT_sb[:, mh, :])
                    hh = scr.tile([P, BL], bf16, name="hh", tag="hh")
                    S.activation(hh, hT_sb[:, mh, :], AF.Square)
                    if mh == 0:
                        V.tensor_copy(x2a, hh)
                    else:
                        V.tensor_add(x2a, x2a, hh)
                # x2 -> batch-major: out[b,0] = sum_p x2a[p, blk*128+b]
                for blk in range(NB):
                    nc.tensor.matmul(stat_ps[:, blk:blk + 1],
                                     lhsT=x2a[:, blk * P:(blk + 1) * P],
                                     rhs=onesb, start=True, stop=True,
                                     skip_group_check=True)

            # ---------- curvature predictor (xT still resident; the
            # AllReduce hides under MM1/MM2) ----------
            with nc.named_scope("cp"):
                cpw1_sb = const.tile([P, KI, 16], bf16, name="cpw1_sb")
                nc.sync.dma_start(out=cpw1_sb,
                                  in_=cpw1_d.rearrange("k p q -> p k q"))
                cpw2_sb = const.tile([16, 1], bf16, name="cpw2_sb")
                nc.sync.dma_start(out=cpw2_sb, in_=cpw2_d[:, :])
                cpb1_sb = const.tile([16, 1], f32, name="cpb1_sb")
                nc.sync.dma_start(out=cpb1_sb, in_=cpb1_d[:, :])
                cpb2_sb = const.tile([1, 1], f32, name="cpb2_sb")
                nc.sync.dma_start(out=cpb2_sb, in_=cpb2_d[:, :])
                cph_sb = cpp.tile([16, BL], bf16, name="cph_sb")
                for ch in range(2):
                    cps = mm.tile([16, 512], f32, name="cps", tag="mm")
                    for ki in range(KI):
                        nc.tensor.matmul(
                            cps, lhsT=cpw1_sb[:, ki, :],
                            rhs=xT_sb[:, ki, ch * 512:(ch + 1) * 512],
                            start=(ki == 0), stop=(ki == KI - 1))
                    S.activation(cph_sb[:, ch * 512:(ch + 1) * 512], cps,
                                 AF.Relu, bias=cpb1_sb)
                sparts = []
                for ch in range(2):
                    c2p = mm.tile([1, 512], f32, name="c2p", tag="mm")
                    nc.tensor.matmul(c2p, lhsT=cpw2_sb,
                                     rhs=cph_sb[:16, ch * 512:(ch + 1) * 512],
                                     start=True, stop=True)
                    cpw = cpp.tile([1, 512], f32, name="cpw", tag="cpw")
                    spart = cpp.tile([1, 1], f32, name=f"spart{ch}",
                                     tag=f"spart{ch}")
                    S.activation(cpw, c2p, AF.Sigmoid, bias=cpb2_sb,
                                 accum_out=spart)
                    sparts.append(spart)
                s_loc = cpp.tile([1, 1], f32, name="s_loc")
                V.tensor_add(s_loc, sparts[0], sparts[1])
                cin = dram.tile([1, 1], f32, name="cin")
                cout = dram.tile([1, 1], f32, name="cout")
                nc.sync.dma_start(out=cin, in_=s_loc)
                nc.gpsimd.collective_compute(
                    "AllReduce", ALU.add,
                    replica_groups=[list(range(N_CORES))],
                    ins=[cin.opt()], outs=[cout.opt()])
                s_b = sc("s_b", (P, 1))
                nc.gpsimd.dma_start(out=s_b, in_=cout.to_broadcast([P, 1]))
                # c = clip(MIN_C + (MAX_C-MIN_C)*mean(c_pred))
                c_b = sc("c_b", (P, 1))
                V.tensor_scalar(out=c_b, in0=s_b,
                                scalar1=(MAX_C - MIN_C) / B_FULL,
                                scalar2=MIN_C, op0=ALU.mult, op1=ALU.add)
                V.tensor_scalar_min(out=c_b, in0=c_b, scalar1=MAX_C)
                V.tensor_scalar_max(out=c_b, in0=c_b, scalar1=MIN_C)
                negc_b = sc("negc_b", (P, 1))
                V.tensor_scalar_mul(out=negc_b, in0=c_b, scalar1=-1.0)
                twoc_b = sc("twoc_b", (P, 1))
                V.tensor_scalar_mul(out=twoc_b, in0=c_b, scalar1=2.0)
                neg2c_b = sc("neg2c_b", (P, 1))
                V.tensor_scalar_mul(out=neg2c_b, in0=c_b, scalar1=-2.0)
                c2_b = sc("c2_b", (P, 1))
                V.tensor_mul(c2_b, c_b, c_b)

            # ---------- MM2: uT = sigmoid(W2.T @ hT) fp8 DoubleRow ----------
            uT_sb = big.tile([P, KH, BL], bf16, name="uT_sb", tag="big")
            inv_s2 = 1.0 / W2S
            with nc.named_scope("mm2"):
                for mh in range(KH):
                    ps = mm.tile([P, BL], f32, name="ps", tag="mm")
                    w2row = wp.tile([P, KH, P], fp8, name="w2row", tag="w2")
                    nc.sync.dma_start(out=w2row, in_=w2_d[mh])
                    for j in range(KH // 2):
                        nc.tensor.matmul(ps[:, 0:512],
                                         lhsT=w2row[:, 2 * j:2 * j + 2, :],
                                         rhs=hTq_sb[:, 2 * j:2 * j + 2, 0:512],
                                         start=(j == 0), stop=(j == KH // 2 - 1),
                                         perf_mode=PM.DoubleRow)
                        nc.tensor.matmul(ps[:, 512:BL],
                                         lhsT=w2row[:, 2 * j:2 * j + 2, :],
                                         rhs=hTq_sb[:, 2 * j:2 * j + 2, 512:BL],
                                         start=(j == 0), stop=(j == KH // 2 - 1),
                                         perf_mode=PM.DoubleRow)
                    if with_b2:
                        S.activation(uT_sb[:, mh, :], ps, AF.Sigmoid,
                                     bias=b2_sb[:, mh:mh + 1], scale=inv_s2)
                    else:
                        S.activation(uT_sb[:, mh, :], ps, AF.Sigmoid,
                                     scale=inv_s2)
                    uu = scr.tile([P, BL], bf16, name="uu", tag="hh")
                    S.activation(uu, uT_sb[:, mh, :], AF.Square)
                    hu = scr.tile([P, BL], bf16, name="hu", tag="hh")
                    V.tensor_mul(hu, hT_sb[:, mh, :], uT_sb[:, mh, :])
                    if mh == 0:
                        V.tensor_copy(y2a, uu)
                        V.tensor_copy(xya, hu)
                    else:
                        V.tensor_add(y2a, y2a, uu)
                        V.tensor_add(xya, xya, hu)
                for blk in range(NB):
                    nc.tensor.matmul(stat_ps[:, 8 + blk:9 + blk],
                                     lhsT=y2a[:, blk * P:(blk + 1) * P],
                                     rhs=onesb, start=True, stop=True,
                                     skip_group_check=True)
                    nc.tensor.matmul(stat_ps[:, 16 + blk:17 + blk],
                                     lhsT=xya[:, blk * P:(blk + 1) * P],
                                     rhs=onesb, start=True, stop=True,
                                     skip_group_check=True)

            # ---------- per-row scalar chain, batch-major [128, 8] ----------
            alpha_b = abp.tile([P, BL], bf16, name="alpha_b")
            beta_b = abp.tile([P, BL], bf16, name="beta_b")
            ab_d = dram.tile([2, BL], bf16, name="ab_d")

            with nc.named_scope("scalars"):
                stats_sb = sc("stats_sb", (P, 24))
                V.tensor_copy(stats_sb, stat_ps)
                x2 = stats_sb[:, 0:8]
                y2 = stats_sb[:, 8:16]
                xy = stats_sb[:, 16:24]
                w = sc("w")
                V.scalar_tensor_tensor(out=w, in0=xy, scalar=-2.0, in1=y2,
                                       op0=ALU.mult, op1=ALU.add)
                A1 = sc("A1")
                V.tensor_scalar(out=A1, in0=w, scalar1=c_b, scalar2=1.0,
                                op0=ALU.mult, op1=ALU.add)
                A2 = sc("A2")
                V.tensor_scalar(out=A2, in0=x2, scalar1=negc_b, scalar2=1.0,
                                op0=ALU.mult, op1=ALU.add)
                p1 = sc("p1")
                V.tensor_mul(p1, x2, y2)
                den = sc("den")
                V.tensor_scalar(out=den, in0=p1, scalar1=c2_b, scalar2=1.0,
                                op0=ALU.mult, op1=ALU.add)
                V.scalar_tensor_tensor(out=den, in0=xy, scalar=neg2c_b, in1=den,
                                       op0=ALU.mult, op1=ALU.add)
                V.tensor_scalar_add(out=den, in0=den, scalar1=EPS)
                D = sc("D")
                V.reciprocal(D, den)
                # ||a||^2 = D^2 (A1^2 x2 - 2 A1 A2 xy + A2^2 y2)
                t1 = sc("t1")
                V.tensor_mul(t1, A1, A1)
                V.tensor_mul(t1, t1, x2)
                t2 = sc("t2")
                V.tensor_mul(t2, A1, A2)
                V.tensor_mul(t2, t2, xy)
                t3 = sc("t3")
                V.tensor_mul(t3, A2, A2)
                V.tensor_mul(t3, t3, y2)
                na2 = sc("na2")
                V.scalar_tensor_tensor(out=na2, in0=t2, scalar=-2.0, in1=t1,
                                       op0=ALU.mult, op1=ALU.add)
                V.tensor_add(na2, na2, t3)
                dsq = sc("dsq")
                V.tensor_mul(dsq, D, D)
                V.tensor_mul(na2, na2, dsq)
                # q = sqrt(c * na2) with one Newton step (ACT sqrt is low precision)
                q2 = sc("q2")
                V.tensor_scalar(out=q2, in0=na2, scalar1=c_b, scalar2=None,
                                op0=ALU.mult)
                q0 = sc("q0")
                S.activation(q0, q2, AF.Sqrt)
                V.tensor_scalar_max(out=q0, in0=q0, scalar1=1e-20)
                r0 = sc("r0")
                V.reciprocal(r0, q0)
                q = sc("q")
                V.tensor_mul(q, q2, r0)
                V.tensor_add(q, q, q0)
                V.tensor_scalar_mul(out=q, in0=q, scalar1=0.5)
                arg = sc("arg")
                V.tensor_scalar_min(out=arg, in0=q, scalar1=1.0 - 1e-5)
                # artanh(arg) = 0.5 ln((1+arg)/(1-arg)); t = tanh(T*artanh)/q
                opp = sc("opp")
                V.tensor_scalar(out=opp, in0=arg, scalar1=-1.0, scalar2=1.0,
                                op0=ALU.mult, op1=ALU.add)
                opn = sc("opn")
                V.tensor_scalar_add(out=opn, in0=arg, scalar1=1.0)
                rr = sc("rr")
                V.reciprocal(rr, opp)
                rat = sc("rat")
                V.tensor_mul(rat, opn, rr)
                lg = sc("lg")
                S.activation(lg, rat, AF.Ln)
                th = sc("th")
                S.activation(th, lg, AF.Tanh, scale=T_CONST * 0.5)
                rq = sc("rq")
                V.reciprocal(rq, q)
                tm = sc("tm")
                V.tensor_mul(tm, th, rq)
                # <h,a> = D (A2 xy - A1 x2)
                s1_ = sc("s1_")
                V.tensor_mul(s1_, A1, x2)
                s2_ = sc("s2_")
                V.tensor_mul(s2_, A2, xy)
                ha = sc("ha")
                V.tensor_sub(ha, s2_, s1_)
                V.tensor_mul(ha, ha, D)
                hm = sc("hm")
                V.tensor_mul(hm, tm, ha)
                tsq = sc("tsq")
                V.tensor_mul(tsq, tm, tm)
                m2 = sc("m2")
                V.tensor_mul(m2, tsq, na2)
                w2s = sc("w2s")
                V.scalar_tensor_tensor(out=w2s, in0=hm, scalar=2.0, in1=m2,
                                       op0=ALU.mult, op1=ALU.add)
                B1 = sc("B1")
                V.tensor_scalar(out=B1, in0=w2s, scalar1=c_b, scalar2=1.0,
                                op0=ALU.mult, op1=ALU.add)
                p2 = sc("p2")
                V.tensor_mul(p2, x2, m2)
                den2 = sc("den2")
                V.tensor_scalar(out=den2, in0=p2, scalar1=c2_b, scalar2=1.0,
                                op0=ALU.mult, op1=ALU.add)
                V.scalar_tensor_tensor(out=den2, in0=hm, scalar=twoc_b, in1=den2,
                                       op0=ALU.mult, op1=ALU.add)
                V.tensor_scalar_add(out=den2, in0=den2, scalar1=EPS)
                D2 = sc("D2")
                V.reciprocal(D2, den2)
                g = sc("g")
                V.tensor_mul(g, A2, tm)
                V.tensor_mul(g, g, D)
                w3 = sc("w3")
                V.tensor_mul(w3, g, A1)
                V.tensor_sub(w3, B1, w3)
                alpha_bm = sc("alpha_bm", (P, 8), bf16)
                V.tensor_mul(alpha_bm, w3, D2)
                w4 = sc("w4")
                V.tensor_mul(w4, g, A2)
                beta_bm = sc("beta_bm", (P, 8), bf16)
                V.tensor_mul(beta_bm, w4, D2)
                # bounce to DRAM in batch-linear order, broadcast back
                nc.sync.dma_start(
                    out=ab_d[0, :].rearrange("(j p) -> p j", p=P), in_=alpha_bm)
                nc.sync.dma_start(
                    out=ab_d[1, :].rearrange("(j p) -> p j", p=P), in_=beta_bm)
                for hsl in (slice(0, 512), slice(512, BL)):
                    nc.gpsimd.dma_start(out=alpha_b[:, hsl],
                                        in_=ab_d[0:1, hsl].to_broadcast([P, 512]))
                    nc.gpsimd.dma_start(out=beta_b[:, hsl],
                                        in_=ab_d[1:2, hsl].to_broadcast([P, 512]))

        # psum pools (mm, stp) released here

        # ---------- z = alpha*h + beta*u (overwrites uT in place),
        # then out = z @ Wo.  Processed in two batch-column halves so the
        # MMo matmuls of half 0 overlap the DVE z-combine of half 1.
        with ExitStack() as ph2:
            mmo = ph2.enter_context(tc.tile_pool(name="mmo", bufs=8,
                                                 space="PSUM"))
            for bg in range(2):
                csl = slice(bg * 512, (bg + 1) * 512)
                with nc.named_scope(f"zcomb{bg}"):
                    for kh in range(KH):
                        t1z = zzp.tile([P, 512], bf16, name="t1z", tag="zz")
                        V.tensor_mul(t1z, hT_sb[:, kh, csl], alpha_b[:, csl])
                        t2z = zzp.tile([P, 512], bf16, name="t2z", tag="zz")
                        V.tensor_mul(t2z, uT_sb[:, kh, csl], beta_b[:, csl])
                        V.tensor_add(uT_sb[:, kh, csl], t1z, t2z)
                with nc.named_scope(f"mmo{bg}"):
                    pso = [mmo.tile([P, 500], f32, name=f"pso{bg}_{i}",
                                    tag="mmo") for i in range(8)]
                    for kh in range(KH):
                        wot = wop.tile([P, OUT], bf16, name="wot", tag="wo")
                        nc.sync.dma_start(out=wot, in_=wo_d[kh])
                        for i in range(4):
                            b = bg * 4 + i
                            for och in range(2):
                                nc.tensor.matmul(
                                    pso[i * 2 + och],
                                    lhsT=uT_sb[:, kh, b * P:(b + 1) * P],
                                    rhs=wot[:, och * 500:(och + 1) * 500],
                                    start=(kh == 0), stop=(kh == KH - 1))
                    for i in range(4):
                        b = bg * 4 + i
                        for och in range(2):
                            ob = outp.tile([P, 500], f32, name="ob", tag="ob")
                            if och == 0:
                                S.copy(ob, pso[i * 2])
                            else:
                                V.tensor_copy(ob, pso[i * 2 + 1])
                            nc.sync.dma_start(
                                out=out_d[b * P:(b + 1) * P,
                                          och * 500:(och + 1) * 500],
                                in_=ob)

    nc.compile()
    return nc


def _get_nc(with_b1, with_b2):
    for k, v in _nc_cache:
        if k == (with_b1, with_b2):
            return v
    nc = _build(with_b1, with_b2)
    _nc_cache.append(((with_b1, with_b2), nc))
    return nc


def kernel(x, W1, b1, W2, b2, Wo, bo, cp_w1, cp_b1, cp_w2, cp_b2,
           _trace=False, _tmpdir=None):
    x = np.asarray(x, dtype=np.float32)
    with_b1 = bool(np.any(b1))
    with_b2 = bool(np.any(b2))
    nc = _get_nc(with_b1, with_b2)

    # w1h[mh, g, p, kk, q] = W1[(g*12+kk)*128+p, mh*128+q]
    w1_t = np.ascontiguousarray(
        np.asarray(W1, np.float32).reshape(KI, P, KH, P).transpose(2, 1, 0, 3)
        .reshape(KH, P, 2, KI // 2, P).transpose(0, 2, 1, 3, 4)
    ).astype(BF)
    # w2r[mh, p, kh, q] = W2[kh*128+p, mh*128+q], fp8 e4m3 scaled x1024
    w2_t = np.ascontiguousarray(
        np.clip(np.asarray(W2, np.float32) * W2S, -E4MAX, E4MAX)
        .reshape(KH, P, KH, P).transpose(2, 1, 0, 3)
    ).astype(E4)
    wo_t = np.asarray(Wo, np.float32).reshape(KH, P, OUT).astype(BF)
    cpw1_t = np.ascontiguousarray(
        np.asarray(cp_w1, np.float32).T.reshape(KI, P, 16)).astype(BF)
    cpw2_t = np.asarray(cp_w2, np.float32).reshape(1, 16).T.astype(BF)
    cpw2_t = np.ascontiguousarray(cpw2_t)
    cpb1_t = np.asarray(cp_b1, np.float32).reshape(16, 1)
    cpb2_t = np.asarray(cp_b2, np.float32).reshape(1, 1)
    b1_t = np.ascontiguousarray(np.asarray(b1, np.float32).reshape(KH, P).T)
    b2_t = np.ascontiguousarray(np.asarray(b2, np.float32).reshape(KH, P).T)

    in_maps = []
    for c in range(N_CORES):
        shard = x[c * BL:(c + 1) * BL]
        xT = np.ascontiguousarray(shard.T).reshape(KI, P, BL).astype(BF)
        m = {"xT": xT, "w1": w1_t, "w2": w2_t, "wo": wo_t,
             "cpw1": cpw1_t, "cpw2": cpw2_t, "cpb1": cpb1_t, "cpb2": cpb2_t}
        if with_b1:
            m["b1"] = b1_t
        if with_b2:
            m["b2"] = b2_t
        in_maps.append(m)

    kw = {}
    if _trace:
        kw = dict(trace=True, tmpdir=_tmpdir or tempfile.mkdtemp(prefix="cdk_"))
    res = run_bass_kernel_spmd(nc, in_maps, list(range(N_CORES)), **kw)

    out = np.concatenate([res.results[c]["out"] for c in range(N_CORES)], axis=0)
    bo = np.asarray(bo, np.float32)
    if np.any(bo):
        out = out + bo
    if _trace:
        kernel._last_result = res
    return out


# revision 8
# speedup vs baseline: 1.2722x; 1.0510x over previous
"""Trainium2 Bass kernel for nn_ConservativeDynamicCurvatureMLP.

Data-parallel over 8 NeuronCores: the batch (8192) is sharded into 8
local shards of 1024 rows; all weights are replicated. The curvature
scalar c_avg couples the shards through a global mean, handled with a
single-scalar AllReduce.

Math (reference):
    h = tanh(x @ W1 + b1)
    u = sigmoid(h @ W2 + b2)
    c = clip(mean(MIN_C + (MAX_C-MIN_C) * sigmoid(relu(x@cp_w1.T+cp_b1)@cp_w2.T+cp_b2)), MIN_C, MAX_C)
    z = poincare_ball_layer(h, u, c, T)
    out = z @ Wo + bo

The poincare layer collapses algebraically to z = alpha(row)*h + beta(row)*u
where alpha/beta are scalar functions of the row statistics
x2=||h||^2, y2=||u||^2, xy=<h,u> and c.  The NaN fallback is measure-zero
and omitted.

Perf notes (measured on hw):
  - With 8 cores active the PE is power-throttled to ~2.0 GHz (HAM state
    31), so bf16 N=512 matmuls pace at ~263 ns.  The only lever is fewer
    PE cycles: MM2 (h@W2, the largest matmul) runs as fp8-e4m3 DoubleRow
    (256-deep contraction per pass, 2x FLOP rate, measured 216 ns/MM at
    2.4 GHz in isolation).  W2 is host-converted to e4m3 * 1024 (clipped
    to +-240, the TRN e4m3 max); the 1/1024 folds into the sigmoid's
    activation scale.  h is cast bf16->fp8 per row-tile on the DVE.
    MM1 / MMo stay bf16: fp8 there pushes rel-err too close to the 2e-2
    gate (simulated 1.96e-2 vs 1.26e-2 for MM2-only).
  - Row statistics go batch-major directly via tiny stationary-squares
    matmuls (lhsT = accumulated squares block, rhs = ones) instead of a
    feature-major ones-reduction + DRAM bounce, shortening the serial
    stats -> alpha/beta tail.  The per-row scalar chain runs once on
    [128, 8] tiles reading stats straight from SBUF.
"""

import tempfile
from contextlib import ExitStack

import numpy as np
import ml_dtypes

import concourse.bass as bass
import concourse.bacc as bacc
import concourse.mybir as mybir
import concourse.tile as tile
from concourse.bass_utils import run_bass_kernel_spmd

P = 128
N_CORES = 8
B_FULL = 8192
BL = B_FULL // N_CORES          # 1024 rows per core
IN = 3072
HID = 4096
OUT = 1000
KI = IN // P                    # 24
KH = HID // P                   # 32
NB = BL // P                    # 8 batch tiles
MIN_C = 0.001 * 0.5
MAX_C = 0.001 * 2.0
T_CONST = 0.7
EPS = 1e-7
W2S = 1024.0                    # host-side fp8 scale for W2
E4MAX = 240.0                   # TRN e4m3 saturation

dt = mybir.dt
AF = mybir.ActivationFunctionType
ALU = mybir.AluOpType
PM = mybir.MatmulPerfMode
BF = ml_dtypes.bfloat16
E4 = ml_dtypes.float8_e4m3

_nc_cache = []
_IDENT = np.eye(P, dtype=ml_dtypes.bfloat16)


def _build(with_b1, with_b2):
    nc = bacc.Bacc("TRN2", target_bir_lowering=False, debug=False,
                   num_devices=N_CORES)

    xT_d = nc.dram_tensor("xT", [KI, P, BL], dt.bfloat16, kind="ExternalInput")
    # W1 rows split in two halves for finer DMA/SBUF: w1h[mh, g, p, kk, q] =
    # W1[(g*12+kk)*128+p, mh*128+q]
    w1_d = nc.dram_tensor("w1", [KH, 2, P, KI // 2, P], dt.bfloat16,
                          kind="ExternalInput")
    # W2 in fp8 (x1024): w2r[mh, p, kh, q] = W2[kh*128+p, mh*128+q]
    w2_d = nc.dram_tensor("w2", [KH, P, KH, P], dt.float8e4, kind="ExternalInput")
    wo_d = nc.dram_tensor("wo", [KH, P, OUT], dt.bfloat16, kind="ExternalInput")
    cpw1_d = nc.dram_tensor("cpw1", [KI, P, 16], dt.bfloat16, kind="ExternalInput")
    cpw2_d = nc.dram_tensor("cpw2", [16, 1], dt.bfloat16, kind="ExternalInput")
    cpb1_d = nc.dram_tensor("cpb1", [16, 1], dt.float32, kind="ExternalInput")
    cpb2_d = nc.dram_tensor("cpb2", [1, 1], dt.float32, kind="ExternalInput")
    ident_d = nc.dram_tensor("ident", [P, P], dt.bfloat16, kind="ExternalInput")
    b1_d = nc.dram_tensor("b1", [P, KH], dt.float32, kind="ExternalInput") if with_b1 else None
    b2_d = nc.dram_tensor("b2", [P, KH], dt.float32, kind="ExternalInput") if with_b2 else None
    out_d = nc.dram_tensor("out", [BL, OUT], dt.float32, kind="ExternalOutput")

    f32 = dt.float32
    bf16 = dt.bfloat16
    fp8 = dt.float8e4

    with tile.TileContext(nc) as tc, ExitStack() as ctx:
        const = ctx.enter_context(tc.tile_pool(name="const", bufs=1))
        big = ctx.enter_context(tc.tile_pool(name="big", bufs=1))
        htp = ctx.enter_context(tc.tile_pool(name="htp", bufs=1))
        hqp = ctx.enter_context(tc.tile_pool(name="hqp", bufs=1))
        wp = ctx.enter_context(tc.tile_pool(name="wp", bufs=2))
        wop = ctx.enter_context(tc.tile_pool(name="wop", bufs=2))
        scr = ctx.enter_context(tc.tile_pool(name="scr", bufs=2))
        zzp = ctx.enter_context(tc.tile_pool(name="zzp", bufs=4))
        sacc = ctx.enter_context(tc.tile_pool(name="sacc", bufs=1))
        abp = ctx.enter_context(tc.tile_pool(name="abp", bufs=1))
        scal = ctx.enter_context(tc.tile_pool(name="scal", bufs=1))
        outp = ctx.enter_context(tc.tile_pool(name="outp", bufs=2))
        cpp = ctx.enter_context(tc.tile_pool(name="cpp", bufs=1))
        dram = ctx.enter_context(tc.tile_pool(name="dram", bufs=1, space="DRAM"))

        V = nc.vector
        S = nc.scalar

        def sc(name, shape=(P, 8), dtype=f32):
            return scal.tile(list(shape), dtype, name=name, tag=name)

        # ---------- persistent activations (feature-major) ----------
        onesb = const.tile([P, 1], bf16, name="onesb")
        nc.vector.memset(onesb, 1.0)
        ident_sb = const.tile([P, P], bf16, name="ident_sb")
        nc.sync.dma_start(out=ident_sb, in_=ident_d[:, :])
        xT_sb = big.tile([P, KI, BL], bf16, name="xT_sb", tag="big",
                         padded_shape=[P, KH, BL])
        # first weight half-row issues on the sync queue before anything else;
        # xT streams on the gpsimd + scalar queues in parallel (the load is
        # HBM-bandwidth-bound, ~35us for 6.3MB with all 8 cores pulling)
        w1row0 = wp.tile([P, KI // 2, P], bf16, name="w1row", tag="w1")
        nc.sync.dma_start(out=w1row0, in_=w1_d[0, 0])
        xq = [nc.gpsimd, nc.scalar]
        for idx, (a, b) in enumerate(((0, 2), (2, 4), (4, 7), (7, 10),
                                      (10, 13), (13, 16), (16, 20), (20, 24))):
            xq[idx % 2].dma_start(
                out=xT_sb[:, a:b, :],
                in_=xT_d[a:b].rearrange("k p b -> p k b"))
        hT_sb = htp.tile([P, KH, BL], bf16, name="hT_sb")
        hTq_sb = hqp.tile([P, KH, BL], fp8, name="hTq_sb")
        if with_b1:
            b1_sb = const.tile([P, KH], f32, name="b1_sb")
            nc.sync.dma_start(out=b1_sb, in_=b1_d[:, :])
        if with_b2:
            b2_sb = const.tile([P, KH], f32, name="b2_sb")
            nc.sync.dma_start(out=b2_sb, in_=b2_d[:, :])

        with ExitStack() as ph1:
            mm = ph1.enter_context(tc.tile_pool(name="mm", bufs=2, space="PSUM"))
            stp = ph1.enter_context(tc.tile_pool(name="stp", bufs=1, space="PSUM"))
            # batch-major stats land here: cols 0:8 x2, 8:16 y2, 16:24 xy
            stat_ps = stp.tile([P, 24], f32, name="stat_ps")
            # scratch targets for HAM warm-keeper matmuls + the a/b transpose
            dum_ps = stp.tile([P, 512], f32, name="dum_ps")
            abT_ps = stp.tile([16, P], bf16, name="abT_ps")

            # ---------- MM1: hT = tanh(W1.T @ xT) (bf16) ----------
            x2a = sacc.tile([P, BL], bf16, name="x2a")
            y2a = sacc.tile([P, BL], bf16, name="y2a")
            xya = sacc.tile([P, BL], bf16, name="xya")
            with nc.named_scope("mm1"):
                for mh in range(KH):
                    ps = mm.tile([P, BL], f32, name="ps", tag="mm")
                    if mh == 0:
                        w1a = w1row0
                    else:
                        w1a = wp.tile([P, KI // 2, P], bf16, name="w1row",
                                      tag="w1")
                        nc.sync.dma_start(out=w1a, in_=w1_d[mh, 0])
                    w1b = wp.tile([P, KI // 2, P], bf16, name="w1row", tag="w1")
                    nc.sync.dma_start(out=w1b, in_=w1_d[mh, 1])
                    for ki in range(KI):
                        wrow = w1a if ki < 12 else w1b
                        kk = ki % 12
                        nc.tensor.matmul(ps[:, 0:512], lhsT=wrow[:, kk, :],
                                         rhs=xT_sb[:, ki, 0:512],
                                         start=(ki == 0), stop=(ki == KI - 1))
                        nc.tensor.matmul(ps[:, 512:BL], lhsT=wrow[:, kk, :],
                                         rhs=xT_sb[:, ki, 512:BL],
                                         start=(ki == 0), stop=(ki == KI - 1))
                    if with_b1:
                        S.activation(hT_sb[:, mh, :], ps, AF.Tanh,
                                     bias=b1_sb[:, mh:mh + 1])
                    else:
                        S.activation(hT_sb[:, mh, :], ps, AF.Tanh)
                    # fp8 copy for MM2's DoubleRow rhs
                    V.tensor_copy(hTq_sb[:, mh, :], hT_sb[:, mh, :])
                    hh = scr.tile([P, BL], bf16, name="hh", tag="hh")
                    S.activation(hh, hT_sb[:, mh, :], AF.Square)
                    if mh == 0:
                        V.tensor_copy(x2a, hh)
                    else:
                        V.tensor_add(x2a, x2a, hh)
                # x2 -> batch-major: out[b,0] = sum_p x2a[p, blk*128+b]
                for blk in range(NB):
                    nc.tensor.matmul(stat_ps[:, blk:blk + 1],
                                     lhsT=x2a[:, blk * P:(blk + 1) * P],
                                     rhs=onesb, start=True, stop=True,
                                     skip_group_check=True)

            # ---------- curvature predictor (xT still resident; the
            # AllReduce hides under MM1/MM2) ----------
            with nc.named_scope("cp"):
                cpw1_sb = const.tile([P, KI, 16], bf16, name="cpw1_sb")
                nc.sync.dma_start(out=cpw1_sb,
                                  in_=cpw1_d.rearrange("k p q -> p k q"))
                cpw2_sb = const.tile([16, 1], bf16, name="cpw2_sb")
                nc.sync.dma_start(out=cpw2_sb, in_=cpw2_d[:, :])
                cpb1_sb = const.tile([16, 1], f32, name="cpb1_sb")
                nc.sync.dma_start(out=cpb1_sb, in_=cpb1_d[:, :])
                cpb2_sb = const.tile([1, 1], f32, name="cpb2_sb")
                nc.sync.dma_start(out=cpb2_sb, in_=cpb2_d[:, :])
                cph_sb = cpp.tile([16, BL], bf16, name="cph_sb")
                for ch in range(2):
                    cps = mm.tile([16, 512], f32, name="cps", tag="mm")
                    for ki in range(KI):
                        nc.tensor.matmul(
                            cps, lhsT=cpw1_sb[:, ki, :],
                            rhs=xT_sb[:, ki, ch * 512:(ch + 1) * 512],
                            start=(ki == 0), stop=(ki == KI - 1))
                    S.activation(cph_sb[:, ch * 512:(ch + 1) * 512], cps,
                                 AF.Relu, bias=cpb1_sb)
                sparts = []
                for ch in range(2):
                    c2p = mm.tile([1, 512], f32, name="c2p", tag="mm")
                    nc.tensor.matmul(c2p, lhsT=cpw2_sb,
                                     rhs=cph_sb[:16, ch * 512:(ch + 1) * 512],
                                     start=True, stop=True)
                    cpw = cpp.tile([1, 512], f32, name="cpw", tag="cpw")
                    spart = cpp.tile([1, 1], f32, name=f"spart{ch}",
                                     tag=f"spart{ch}")
                    S.activation(cpw, c2p, AF.Sigmoid, bias=cpb2_sb,
                                 accum_out=spart)
                    sparts.append(spart)
                s_loc = cpp.tile([1, 1], f32, name="s_loc")
                V.tensor_add(s_loc, sparts[0], sparts[1])
                cin = dram.tile([1, 1], f32, name="cin")
                cout = dram.tile([1, 1], f32, name="cout")
                nc.sync.dma_start(out=cin, in_=s_loc)
                nc.gpsimd.collective_compute(
                    "AllReduce", ALU.add,
                    replica_groups=[list(range(N_CORES))],
                    ins=[cin.opt()], outs=[cout.opt()])
                s_b = sc("s_b", (P, 1))
                nc.gpsimd.dma_start(out=s_b, in_=cout.to_broadcast([P, 1]))
                # c = clip(MIN_C + (MAX_C-MIN_C)*mean(c_pred))
                c_b = sc("c_b", (P, 1))
                V.tensor_scalar(out=c_b, in0=s_b,
                                scalar1=(MAX_C - MIN_C) / B_FULL,
                                scalar2=MIN_C, op0=ALU.mult, op1=ALU.add)
                V.tensor_scalar_min(out=c_b, in0=c_b, scalar1=MAX_C)
                V.tensor_scalar_max(out=c_b, in0=c_b, scalar1=MIN_C)
                negc_b = sc("negc_b", (P, 1))
                V.tensor_scalar_mul(out=negc_b, in0=c_b, scalar1=-1.0)
                twoc_b = sc("twoc_b", (P, 1))
                V.tensor_scalar_mul(out=twoc_b, in0=c_b, scalar1=2.0)
                neg2c_b = sc("neg2c_b", (P, 1))
                V.tensor_scalar_mul(out=neg2c_b, in0=c_b, scalar1=-2.0)
                c2_b = sc("c2_b", (P, 1))
                V.tensor_mul(c2_b, c_b, c_b)

            # ---------- MM2: uT = sigmoid(W2.T @ hT) fp8 DoubleRow ----------
            uT_sb = big.tile([P, KH, BL], bf16, name="uT_sb", tag="big")
            inv_s2 = 1.0 / W2S
            with nc.named_scope("mm2"):
                for mh in range(KH):
                    ps = mm.tile([P, BL], f32, name="ps", tag="mm")
                    w2row = wp.tile([P, KH, P], fp8, name="w2row", tag="w2")
                    nc.sync.dma_start(out=w2row, in_=w2_d[mh])
                    for j in range(KH // 2):
                        nc.tensor.matmul(ps[:, 0:512],
                                         lhsT=w2row[:, 2 * j:2 * j + 2, :],
                                         rhs=hTq_sb[:, 2 * j:2 * j + 2, 0:512],
                                         start=(j == 0), stop=(j == KH // 2 - 1),
                                         perf_mode=PM.DoubleRow)
                        nc.tensor.matmul(ps[:, 512:BL],
                                         lhsT=w2row[:, 2 * j:2 * j + 2, :],
                                         rhs=hTq_sb[:, 2 * j:2 * j + 2, 512:BL],
                                         start=(j == 0), stop=(j == KH // 2 - 1),
                                         perf_mode=PM.DoubleRow)
                    if with_b2:
                        S.activation(uT_sb[:, mh, :], ps, AF.Sigmoid,
                                     bias=b2_sb[:, mh:mh + 1], scale=inv_s2)
                    else:
                        S.activation(uT_sb[:, mh, :], ps, AF.Sigmoid,
                                     scale=inv_s2)
                    uu = scr.tile([P, BL], bf16, name="uu", tag="hh")
                    S.activation(uu, uT_sb[:, mh, :], AF.Square)
                    hu = scr.tile([P, BL], bf16, name="hu", tag="hh")
                    V.tensor_mul(hu, hT_sb[:, mh, :], uT_sb[:, mh, :])
                    if mh == 0:
                        V.tensor_copy(y2a, uu)
                        V.tensor_copy(xya, hu)
                    else:
                        V.tensor_add(y2a, y2a, uu)
                        V.tensor_add(xya, xya, hu)
                for blk in range(NB):
                    nc.tensor.matmul(stat_ps[:, 8 + blk:9 + blk],
                                     lhsT=y2a[:, blk * P:(blk + 1) * P],
                                     rhs=onesb, start=True, stop=True,
                                     skip_group_check=True)
                    nc.tensor.matmul(stat_ps[:, 16 + blk:17 + blk],
                                     lhsT=xya[:, blk * P:(blk + 1) * P],
                                     rhs=onesb, start=True, stop=True,
                                     skip_group_check=True)

            # ---------- per-row scalar chain, batch-major [128, 8] ----------
            alpha_b = abp.tile([P, BL], bf16, name="alpha_b")
            beta_b = abp.tile([P, BL], bf16, name="beta_b")
            ab_d = dram.tile([2, BL], bf16, name="ab_d")

            with nc.named_scope("scalars"):
                stats_sb = sc("stats_sb", (P, 24))
                V.tensor_copy(stats_sb, stat_ps)
                x2 = stats_sb[:, 0:8]
                y2 = stats_sb[:, 8:16]
                xy = stats_sb[:, 16:24]
                w = sc("w")
                V.scalar_tensor_tensor(out=w, in0=xy, scalar=-2.0, in1=y2,
                                       op0=ALU.mult, op1=ALU.add)
                A1 = sc("A1")
                V.tensor_scalar(out=A1, in0=w, scalar1=c_b, scalar2=1.0,
                                op0=ALU.mult, op1=ALU.add)
                A2 = sc("A2")
                V.tensor_scalar(out=A2, in0=x2, scalar1=negc_b, scalar2=1.0,
                                op0=ALU.mult, op1=ALU.add)
                p1 = sc("p1")
                V.tensor_mul(p1, x2, y2)
                den = sc("den")
                V.tensor_scalar(out=den, in0=p1, scalar1=c2_b, scalar2=1.0,
                                op0=ALU.mult, op1=ALU.add)
                V.scalar_tensor_tensor(out=den, in0=xy, scalar=neg2c_b, in1=den,
                                       op0=ALU.mult, op1=ALU.add)
                V.tensor_scalar_add(out=den, in0=den, scalar1=EPS)
                D = sc("D")
                V.reciprocal(D, den)
                # ||a||^2 = D^2 (A1^2 x2 - 2 A1 A2 xy + A2^2 y2)
                t1 = sc("t1")
                V.tensor_mul(t1, A1, A1)
                V.tensor_mul(t1, t1, x2)
                t2 = sc("t2")
                V.tensor_mul(t2, A1, A2)
                V.tensor_mul(t2, t2, xy)
                t3 = sc("t3")
                V.tensor_mul(t3, A2, A2)
                V.tensor_mul(t3, t3, y2)
                na2 = sc("na2")
                V.scalar_tensor_tensor(out=na2, in0=t2, scalar=-2.0, in1=t1,
                                       op0=ALU.mult, op1=ALU.add)
                V.tensor_add(na2, na2, t3)
                dsq = sc("dsq")
                V.tensor_mul(dsq, D, D)
                V.tensor_mul(na2, na2, dsq)
                # q = sqrt(c * na2) with one Newton step (ACT sqrt is low precision)
                q2 = sc("q2")
                V.tensor_scalar(out=q2, in0=na2, scalar1=c_b, scalar2=None,
                                op0=ALU.mult)
                q0 = sc("q0")
                S.activation(q0, q2, AF.Sqrt)
                V.tensor_scalar_max(out=q0, in0=q0, scalar1=1e-20)
                r0 = sc("r0")
                V.reciprocal(r0, q0)
                q = sc("q")
                V.tensor_mul(q, q2, r0)
                V.tensor_add(q, q, q0)
                V.tensor_scalar_mul(out=q, in0=q, scalar1=0.5)
                arg = sc("arg")
                V.tensor_scalar_min(out=arg, in0=q, scalar1=1.0 - 1e-5)
                # artanh(arg) = 0.5 ln((1+arg)/(1-arg)); t = tanh(T*artanh)/q
                opp = sc("opp")
                V.tensor_scalar(out=opp, in0=arg, scalar1=-1.0, scalar2=1.0,
                                op0=ALU.mult, op1=ALU.add)
                opn = sc("opn")
                V.tensor_scalar_add(out=opn, in0=arg, scalar1=1.0)
                rr = sc("rr")
                V.reciprocal(rr, opp)
                rat = sc("rat")
                V.tensor_mul(rat, opn, rr)
                lg = sc("lg")
                S.activation(lg, rat, AF.Ln)
                th = sc("th")
                S.activation(th, lg, AF.Tanh, scale=T_CONST * 0.5)
                rq = sc("rq")
                V.reciprocal(rq, q)
                tm = sc("tm")
                V.tensor_mul(tm, th, rq)
                # <h,a> = D (A2 xy - A1 x2)
                s1_ = sc("s1_")
                V.tensor_mul(s1_, A1, x2)
                s2_ = sc("s2_")
                V.tensor_mul(s2_, A2, xy)
                ha = sc("ha")
                V.tensor_sub(ha, s2_, s1_)
                V.tensor_mul(ha, ha, D)
                hm = sc("hm")
                V.tensor_mul(hm, tm, ha)
                tsq = sc("tsq")
                V.tensor_mul(tsq, tm, tm)
                m2 = sc("m2")
                V.tensor_mul(m2, tsq, na2)
                w2s = sc("w2s")
                V.scalar_tensor_tensor(out=w2s, in0=hm, scalar=2.0, in1=m2,
                                       op0=ALU.mult, op1=ALU.add)
                B1 = sc("B1")
                V.tensor_scalar(out=B1, in0=w2s, scalar1=c_b, scalar2=1.0,
                                op0=ALU.mult, op1=ALU.add)
                p2 = sc("p2")
                V.tensor_mul(p2, x2, m2)
                den2 = sc("den2")
                V.tensor_scalar(out=den2, in0=p2, scalar1=c2_b, scalar2=1.0,
                                op0=ALU.mult, op1=ALU.add)
                V.scalar_tensor_tensor(out=den2, in0=hm, scalar=twoc_b, in1=den2,
                                       op0=ALU.mult, op1=ALU.add)
                V.tensor_scalar_add(out=den2, in0=den2, scalar1=EPS)
                D2 = sc("D2")
                V.reciprocal(D2, den2)
                g = sc("g")
                V.tensor_mul(g, A2, tm)
                V.tensor_mul(g, g, D)
                w3 = sc("w3")
                V.tensor_mul(w3, g, A1)
                V.tensor_sub(w3, B1, w3)
                # alpha -> cols 0:8, beta -> cols 8:16 of one bf16 tile; a PE
                # transpose then yields batch-linear rows for a fast DMA
                # bounce (8 contiguous 256B descriptors instead of a 2-byte
                # element scatter).
                ab_bm = sc("ab_bm", (P, 16), bf16)
                V.tensor_mul(ab_bm[:, 0:8], w3, D2)
                w4 = sc("w4")
                V.tensor_mul(w4, g, A2)
                V.tensor_mul(ab_bm[:, 8:16], w4, D2)
                # keep the PE busy through the scalar chain so HAM doesn't
                # re-throttle the clock before the MMo matmuls (idle > ~3.4us
                # drops the PE to 1.2 GHz)
                for _ in range(26):
                    nc.tensor.matmul(dum_ps, lhsT=ident_sb,
                                     rhs=hT_sb[:, 0, 0:512],
                                     start=True, stop=True,
                                     skip_group_check=True)
                nc.tensor.transpose(abT_ps, ab_bm, ident_sb)
                for _ in range(8):
                    nc.tensor.matmul(dum_ps, lhsT=ident_sb,
                                     rhs=hT_sb[:, 1, 0:512],
                                     start=True, stop=True,
                                     skip_group_check=True)
                # anchor the warm-keeper matmuls against dead-code elimination
                dum_sb = sc("dum_sb", (1, 1))
                V.tensor_copy(dum_sb, dum_ps[0:1, 0:1])
                dum_d = dram.tile([1, 1], f32, name="dum_d")
                nc.sync.dma_start(out=dum_d, in_=dum_sb)
                abT_sb = sc("abT_sb", (16, P), bf16)
                V.tensor_copy(abT_sb, abT_ps)
                nc.sync.dma_start(out=ab_d[0, :].rearrange("(j b) -> j b", j=8),
                                  in_=abT_sb[0:8, :])
                nc.sync.dma_start(out=ab_d[1, :].rearrange("(j b) -> j b", j=8),
                                  in_=abT_sb[8:16, :])
                for hsl in (slice(0, 512), slice(512, BL)):
                    nc.gpsimd.dma_start(out=alpha_b[:, hsl],
                                        in_=ab_d[0:1, hsl].to_broadcast([P, 512]))
                    nc.scalar.dma_start(out=beta_b[:, hsl],
                                        in_=ab_d[1:2, hsl].to_broadcast([P, 512]))

        # psum pools (mm, stp) released here

        # ---------- z = alpha*h + beta*u (overwrites uT in place),
        # then out = z @ Wo.  Processed in two batch-column halves so the
        # MMo matmuls of half 0 overlap the DVE z-combine of half 1.
        with ExitStack() as ph2:
            mmo = ph2.enter_context(tc.tile_pool(name="mmo", bufs=8,
                                                 space="PSUM"))
            for bg in range(2):
                csl = slice(bg * 512, (bg + 1) * 512)
                with nc.named_scope(f"zcomb{bg}"):
                    for kh in range(KH):
                        t1z = zzp.tile([P, 512], bf16, name="t1z", tag="zz")
                        V.tensor_mul(t1z, hT_sb[:, kh, csl], alpha_b[:, csl])
                        t2z = zzp.tile([P, 512], bf16, name="t2z", tag="zz")
                        V.tensor_mul(t2z, uT_sb[:, kh, csl], beta_b[:, csl])
                        V.tensor_add(uT_sb[:, kh, csl], t1z, t2z)
                with nc.named_scope(f"mmo{bg}"):
                    pso = [mmo.tile([P, 500], f32, name=f"pso{bg}_{i}",
                                    tag="mmo") for i in range(8)]
                    for kh in range(KH):
                        wot = wop.tile([P, OUT], bf16, name="wot", tag="wo")
                        nc.sync.dma_start(out=wot, in_=wo_d[kh])
                        for i in range(4):
                            b = bg * 4 + i
                            for och in range(2):
                                nc.tensor.matmul(
                                    pso[i * 2 + och],
                                    lhsT=uT_sb[:, kh, b * P:(b + 1) * P],
                                    rhs=wot[:, och * 500:(och + 1) * 500],
                                    start=(kh == 0), stop=(kh == KH - 1))
                    for i in range(4):
                        b = bg * 4 + i
                        for och in range(2):
                            ob = outp.tile([P, 500], f32, name="ob", tag="ob")
                            if och == 0:
                                S.copy(ob, pso[i * 2])
                            else:
                                V.tensor_copy(ob, pso[i * 2 + 1])
                            nc.sync.dma_start(
                                out=out_d[b * P:(b + 1) * P,
                                          och * 500:(och + 1) * 500],
                                in_=ob)

    nc.compile()
    return nc


def _get_nc(with_b1, with_b2):
    for k, v in _nc_cache:
        if k == (with_b1, with_b2):
            return v
    nc = _build(with_b1, with_b2)
    _nc_cache.append(((with_b1, with_b2), nc))
    return nc


def kernel(x, W1, b1, W2, b2, Wo, bo, cp_w1, cp_b1, cp_w2, cp_b2,
           _trace=False, _tmpdir=None):
    x = np.asarray(x, dtype=np.float32)
    with_b1 = bool(np.any(b1))
    with_b2 = bool(np.any(b2))
    nc = _get_nc(with_b1, with_b2)

    # w1h[mh, g, p, kk, q] = W1[(g*12+kk)*128+p, mh*128+q]
    w1_t = np.ascontiguousarray(
        np.asarray(W1, np.float32).reshape(KI, P, KH, P).transpose(2, 1, 0, 3)
        .reshape(KH, P, 2, KI // 2, P).transpose(0, 2, 1, 3, 4)
    ).astype(BF)
    # w2r[mh, p, kh, q] = W2[kh*128+p, mh*128+q], fp8 e4m3 scaled x1024
    w2_t = np.ascontiguousarray(
        np.clip(np.asarray(W2, np.float32) * W2S, -E4MAX, E4MAX)
        .reshape(KH, P, KH, P).transpose(2, 1, 0, 3)
    ).astype(E4)
    wo_t = np.asarray(Wo, np.float32).reshape(KH, P, OUT).astype(BF)
    cpw1_t = np.ascontiguousarray(
        np.asarray(cp_w1, np.float32).T.reshape(KI, P, 16)).astype(BF)
    cpw2_t = np.asarray(cp_w2, np.float32).reshape(1, 16).T.astype(BF)
    cpw2_t = np.ascontiguousarray(cpw2_t)
    cpb1_t = np.asarray(cp_b1, np.float32).reshape(16, 1)
    cpb2_t = np.asarray(cp_b2, np.float32).reshape(1, 1)
    b1_t = np.ascontiguousarray(np.asarray(b1, np.float32).reshape(KH, P).T)
    b2_t = np.ascontiguousarray(np.asarray(b2, np.float32).reshape(KH, P).T)

    in_maps = []
    for c in range(N_CORES):
        shard = x[c * BL:(c + 1) * BL]
        xT = np.ascontiguousarray(shard.T).reshape(KI, P, BL).astype(BF)
        m = {"xT": xT, "w1": w1_t, "w2": w2_t, "wo": wo_t,
             "cpw1": cpw1_t, "cpw2": cpw2_t, "cpb1": cpb1_t, "cpb2": cpb2_t,
             "ident": _IDENT}
        if with_b1:
            m["b1"] = b1_t
        if with_b2:
            m["b2"] = b2_t
        in_maps.append(m)

    kw = {}
    if _trace:
        kw = dict(trace=True, tmpdir=_tmpdir or tempfile.mkdtemp(prefix="cdk_"))
    res = run_bass_kernel_spmd(nc, in_maps, list(range(N_CORES)), **kw)

    out = np.concatenate([res.results[c]["out"] for c in range(N_CORES)], axis=0)
    bo = np.asarray(bo, np.float32)
    if np.any(bo):
        out = out + bo
    if _trace:
        kernel._last_result = res
    return out


# revision 11
# speedup vs baseline: 1.2724x; 1.0002x over previous
"""Trainium2 Bass kernel for nn_ConservativeDynamicCurvatureMLP.

Data-parallel over 8 NeuronCores: the batch (8192) is sharded into 8
local shards of 1024 rows; all weights are replicated. The curvature
scalar c_avg couples the shards through a global mean, handled with a
single-scalar AllReduce.

Math (reference):
    h = tanh(x @ W1 + b1)
    u = sigmoid(h @ W2 + b2)
    c = clip(mean(MIN_C + (MAX_C-MIN_C) * sigmoid(relu(x@cp_w1.T+cp_b1)@cp_w2.T+cp_b2)), MIN_C, MAX_C)
    z = poincare_ball_layer(h, u, c, T)
    out = z @ Wo + bo

The poincare layer collapses algebraically to z = alpha(row)*h + beta(row)*u
where alpha/beta are scalar functions of the row statistics
x2=||h||^2, y2=||u||^2, xy=<h,u> and c.  The NaN fallback is measure-zero
and omitted.

Perf notes (measured on hw):
  - With 8 cores active the PE is power-throttled to ~2.0 GHz (HAM state
    31), so bf16 N=512 matmuls pace at ~263 ns.  The only lever is fewer
    PE cycles: MM2 (h@W2, the largest matmul) runs as fp8-e4m3 DoubleRow
    (256-deep contraction per pass, 2x FLOP rate, measured 216 ns/MM at
    2.4 GHz in isolation).  W2 is host-converted to e4m3 * 1024 (clipped
    to +-240, the TRN e4m3 max); the 1/1024 folds into the sigmoid's
    activation scale.  h is cast bf16->fp8 per row-tile on the DVE.
    MM1 / MMo stay bf16: fp8 there pushes rel-err too close to the 2e-2
    gate (simulated 1.96e-2 vs 1.26e-2 for MM2-only).
  - Row statistics go batch-major directly via tiny stationary-squares
    matmuls (lhsT = accumulated squares block, rhs = ones) instead of a
    feature-major ones-reduction + DRAM bounce, shortening the serial
    stats -> alpha/beta tail.  The per-row scalar chain runs once on
    [128, 8] tiles reading stats straight from SBUF.
"""

import tempfile
from contextlib import ExitStack

import numpy as np
import ml_dtypes

import concourse.bass as bass
import concourse.bacc as bacc
import concourse.mybir as mybir
import concourse.tile as tile
from concourse.bass_utils import run_bass_kernel_spmd

P = 128
N_CORES = 8
B_FULL = 8192
BL = B_FULL // N_CORES          # 1024 rows per core
IN = 3072
HID = 4096
OUT = 1000
KI = IN // P                    # 24
KH = HID // P                   # 32
NB = BL // P                    # 8 batch tiles
MIN_C = 0.001 * 0.5
MAX_C = 0.001 * 2.0
T_CONST = 0.7
EPS = 1e-7
W2S = 1024.0                    # host-side fp8 scale for W2
E4MAX = 240.0                   # TRN e4m3 saturation

dt = mybir.dt
AF = mybir.ActivationFunctionType
ALU = mybir.AluOpType
PM = mybir.MatmulPerfMode
BF = ml_dtypes.bfloat16
E4 = ml_dtypes.float8_e4m3

_nc_cache = []
_IDENT = np.eye(P, dtype=ml_dtypes.bfloat16)


def _build(with_b1, with_b2):
    nc = bacc.Bacc("TRN2", target_bir_lowering=False, debug=False,
                   num_devices=N_CORES)

    xT_d = nc.dram_tensor("xT", [KI, P, BL], dt.bfloat16, kind="ExternalInput")
    # W1 rows split in two halves for finer DMA/SBUF: w1h[mh, g, p, kk, q] =
    # W1[(g*12+kk)*128+p, mh*128+q]
    w1_d = nc.dram_tensor("w1", [KH, 2, P, KI // 2, P], dt.bfloat16,
                          kind="ExternalInput")
    # W2 in fp8 (x1024): w2r[mh, p, kh, q] = W2[kh*128+p, mh*128+q]
    w2_d = nc.dram_tensor("w2", [KH, P, KH, P], dt.float8e4, kind="ExternalInput")
    wo_d = nc.dram_tensor("wo", [KH, P, OUT], dt.bfloat16, kind="ExternalInput")
    cpw1_d = nc.dram_tensor("cpw1", [KI, P, 16], dt.bfloat16, kind="ExternalInput")
    cpw2_d = nc.dram_tensor("cpw2", [16, 1], dt.bfloat16, kind="ExternalInput")
    cpb1_d = nc.dram_tensor("cpb1", [16, 1], dt.float32, kind="ExternalInput")
    cpb2_d = nc.dram_tensor("cpb2", [1, 1], dt.float32, kind="ExternalInput")
    ident_d = nc.dram_tensor("ident", [P, P], dt.bfloat16, kind="ExternalInput")
    b1_d = nc.dram_tensor("b1", [P, KH], dt.float32, kind="ExternalInput") if with_b1 else None
    b2_d = nc.dram_tensor("b2", [P, KH], dt.float32, kind="ExternalInput") if with_b2 else None
    out_d = nc.dram_tensor("out", [BL, OUT], dt.float32, kind="ExternalOutput")

    f32 = dt.float32
    bf16 = dt.bfloat16
    fp8 = dt.float8e4

    with tile.TileContext(nc) as tc, ExitStack() as ctx:
        const = ctx.enter_context(tc.tile_pool(name="const", bufs=1))
        big = ctx.enter_context(tc.tile_pool(name="big", bufs=1))
        htp = ctx.enter_context(tc.tile_pool(name="htp", bufs=1))
        hqp = ctx.enter_context(tc.tile_pool(name="hqp", bufs=1))
        wp = ctx.enter_context(tc.tile_pool(name="wp", bufs=2))
        wop = ctx.enter_context(tc.tile_pool(name="wop", bufs=2))
        scr = ctx.enter_context(tc.tile_pool(name="scr", bufs=2))
        zzp = ctx.enter_context(tc.tile_pool(name="zzp", bufs=4))
        sacc = ctx.enter_context(tc.tile_pool(name="sacc", bufs=1))
        abp = ctx.enter_context(tc.tile_pool(name="abp", bufs=1))
        scal = ctx.enter_context(tc.tile_pool(name="scal", bufs=1))
        outp = ctx.enter_context(tc.tile_pool(name="outp", bufs=2))
        cpp = ctx.enter_context(tc.tile_pool(name="cpp", bufs=1))
        dram = ctx.enter_context(tc.tile_pool(name="dram", bufs=1, space="DRAM"))

        V = nc.vector
        S = nc.scalar

        def sc(name, shape=(P, 8), dtype=f32):
            return scal.tile(list(shape), dtype, name=name, tag=name)

        # ---------- persistent activations (feature-major) ----------
        onesb = const.tile([P, 1], bf16, name="onesb")
        nc.vector.memset(onesb, 1.0)
        xT_sb = big.tile([P, KI, BL], bf16, name="xT_sb", tag="big",
                         padded_shape=[P, KH, BL])
        # first weight half-row issues on the sync queue before anything else;
        # xT streams on the gpsimd + scalar queues in parallel (the load is
        # HBM-bandwidth-bound, ~35us for 6.3MB with all 8 cores pulling)
        w1row0 = wp.tile([P, KI // 2, P], bf16, name="w1row", tag="w1")
        nc.sync.dma_start(out=w1row0, in_=w1_d[0, 0])
        xq = [nc.gpsimd, nc.scalar]
        for idx, (a, b) in enumerate(((0, 2), (2, 4), (4, 7), (7, 10),
                                      (10, 13), (13, 16), (16, 20), (20, 24))):
            xq[idx % 2].dma_start(
                out=xT_sb[:, a:b, :],
                in_=xT_d[a:b].rearrange("k p b -> p k b"))
        ident_sb = const.tile([P, P], bf16, name="ident_sb")
        nc.sync.dma_start(out=ident_sb, in_=ident_d[:, :])
        hT_sb = htp.tile([P, KH, BL], bf16, name="hT_sb")
        hTq_sb = hqp.tile([P, KH, BL], fp8, name="hTq_sb")
        if with_b1:
            b1_sb = const.tile([P, KH], f32, name="b1_sb")
            nc.sync.dma_start(out=b1_sb, in_=b1_d[:, :])
        if with_b2:
            b2_sb = const.tile([P, KH], f32, name="b2_sb")
            nc.sync.dma_start(out=b2_sb, in_=b2_d[:, :])

        with ExitStack() as ph1:
            mm = ph1.enter_context(tc.tile_pool(name="mm", bufs=2, space="PSUM"))
            stp = ph1.enter_context(tc.tile_pool(name="stp", bufs=1, space="PSUM"))
            # batch-major stats land here: cols 0:8 x2, 8:16 y2, 16:24 xy
            stat_ps = stp.tile([P, 24], f32, name="stat_ps")
            # scratch targets for HAM warm-keeper matmuls + the a/b transpose
            dum_ps = stp.tile([P, 512], f32, name="dum_ps")
            abT_ps = stp.tile([16, P], bf16, name="abT_ps")

            # ---------- MM1: hT = tanh(W1.T @ xT) (bf16) ----------
            x2a = sacc.tile([P, BL], bf16, name="x2a")
            y2a = sacc.tile([P, BL], bf16, name="y2a")
            xya = sacc.tile([P, BL], bf16, name="xya")
            with nc.named_scope("mm1"):
                for mh in range(KH):
                    ps = mm.tile([P, BL], f32, name="ps", tag="mm")
                    if mh == 0:
                        w1a = w1row0
                    else:
                        w1a = wp.tile([P, KI // 2, P], bf16, name="w1row",
                                      tag="w1")
                        nc.sync.dma_start(out=w1a, in_=w1_d[mh, 0])
                    w1b = wp.tile([P, KI // 2, P], bf16, name="w1row", tag="w1")
                    nc.sync.dma_start(out=w1b, in_=w1_d[mh, 1])
                    for ki in range(KI):
                        wrow = w1a if ki < 12 else w1b
                        kk = ki % 12
                        nc.tensor.matmul(ps[:, 0:512], lhsT=wrow[:, kk, :],
                                         rhs=xT_sb[:, ki, 0:512],
                                         start=(ki == 0), stop=(ki == KI - 1))
                        nc.tensor.matmul(ps[:, 512:BL], lhsT=wrow[:, kk, :],
                                         rhs=xT_sb[:, ki, 512:BL],
                                         start=(ki == 0), stop=(ki == KI - 1))
                    if with_b1:
                        S.activation(hT_sb[:, mh, :], ps, AF.Tanh,
                                     bias=b1_sb[:, mh:mh + 1])
                    else:
                        S.activation(hT_sb[:, mh, :], ps, AF.Tanh)
                    # fp8 copy for MM2's DoubleRow rhs
                    V.tensor_copy(hTq_sb[:, mh, :], hT_sb[:, mh, :])
                    hh = scr.tile([P, BL], bf16, name="hh", tag="hh")
                    S.activation(hh, hT_sb[:, mh, :], AF.Square)
                    if mh == 0:
                        V.tensor_copy(x2a, hh)
                    else:
                        V.tensor_add(x2a, x2a, hh)
                # x2 -> batch-major: out[b,0] = sum_p x2a[p, blk*128+b]
                for blk in range(NB):
                    nc.tensor.matmul(stat_ps[:, blk:blk + 1],
                                     lhsT=x2a[:, blk * P:(blk + 1) * P],
                                     rhs=onesb, start=True, stop=True,
                                     skip_group_check=True)

            # ---------- curvature predictor (xT still resident; the
            # AllReduce hides under MM1/MM2) ----------
            with nc.named_scope("cp"):
                cpw1_sb = const.tile([P, KI, 16], bf16, name="cpw1_sb")
                nc.sync.dma_start(out=cpw1_sb,
                                  in_=cpw1_d.rearrange("k p q -> p k q"))
                cpw2_sb = const.tile([16, 1], bf16, name="cpw2_sb")
                nc.sync.dma_start(out=cpw2_sb, in_=cpw2_d[:, :])
                cpb1_sb = const.tile([16, 1], f32, name="cpb1_sb")
                nc.sync.dma_start(out=cpb1_sb, in_=cpb1_d[:, :])
                cpb2_sb = const.tile([1, 1], f32, name="cpb2_sb")
                nc.sync.dma_start(out=cpb2_sb, in_=cpb2_d[:, :])
                cph_sb = cpp.tile([16, BL], bf16, name="cph_sb")
                for ch in range(2):
                    cps = mm.tile([16, 512], f32, name="cps", tag="mm")
                    for ki in range(KI):
                        nc.tensor.matmul(
                            cps, lhsT=cpw1_sb[:, ki, :],
                            rhs=xT_sb[:, ki, ch * 512:(ch + 1) * 512],
                            start=(ki == 0), stop=(ki == KI - 1))
                    S.activation(cph_sb[:, ch * 512:(ch + 1) * 512], cps,
                                 AF.Relu, bias=cpb1_sb)
                sparts = []
                for ch in range(2):
                    c2p = mm.tile([1, 512], f32, name="c2p", tag="mm")
                    nc.tensor.matmul(c2p, lhsT=cpw2_sb,
                                     rhs=cph_sb[:16, ch * 512:(ch + 1) * 512],
                                     start=True, stop=True)
                    cpw = cpp.tile([1, 512], f32, name="cpw", tag="cpw")
                    spart = cpp.tile([1, 1], f32, name=f"spart{ch}",
                                     tag=f"spart{ch}")
                    S.activation(cpw, c2p, AF.Sigmoid, bias=cpb2_sb,
                                 accum_out=spart)
                    sparts.append(spart)
                s_loc = cpp.tile([1, 1], f32, name="s_loc")
                V.tensor_add(s_loc, sparts[0], sparts[1])
                cin = dram.tile([1, 1], f32, name="cin")
                cout = dram.tile([1, 1], f32, name="cout")
                nc.sync.dma_start(out=cin, in_=s_loc)
                nc.gpsimd.collective_compute(
                    "AllReduce", ALU.add,
                    replica_groups=[list(range(N_CORES))],
                    ins=[cin.opt()], outs=[cout.opt()])
                s_b = sc("s_b", (P, 1))
                nc.gpsimd.dma_start(out=s_b, in_=cout.to_broadcast([P, 1]))
                # c = clip(MIN_C + (MAX_C-MIN_C)*mean(c_pred))
                c_b = sc("c_b", (P, 1))
                V.tensor_scalar(out=c_b, in0=s_b,
                                scalar1=(MAX_C - MIN_C) / B_FULL,
                                scalar2=MIN_C, op0=ALU.mult, op1=ALU.add)
                V.tensor_scalar_min(out=c_b, in0=c_b, scalar1=MAX_C)
                V.tensor_scalar_max(out=c_b, in0=c_b, scalar1=MIN_C)
                negc_b = sc("negc_b", (P, 1))
                V.tensor_scalar_mul(out=negc_b, in0=c_b, scalar1=-1.0)
                twoc_b = sc("twoc_b", (P, 1))
                V.tensor_scalar_mul(out=twoc_b, in0=c_b, scalar1=2.0)
                neg2c_b = sc("neg2c_b", (P, 1))
                V.tensor_scalar_mul(out=neg2c_b, in0=c_b, scalar1=-2.0)
                c2_b = sc("c2_b", (P, 1))
                V.tensor_mul(c2_b, c_b, c_b)

            # ---------- MM2: uT = sigmoid(W2.T @ hT) fp8 DoubleRow ----------
            uT_sb = big.tile([P, KH, BL], bf16, name="uT_sb", tag="big")
            inv_s2 = 1.0 / W2S
            with nc.named_scope("mm2"):
                for mh in range(KH):
                    ps = mm.tile([P, BL], f32, name="ps", tag="mm")
                    w2row = wp.tile([P, KH, P], fp8, name="w2row", tag="w2")
                    nc.sync.dma_start(out=w2row, in_=w2_d[mh])
                    for j in range(KH // 2):
                        nc.tensor.matmul(ps[:, 0:512],
                                         lhsT=w2row[:, 2 * j:2 * j + 2, :],
                                         rhs=hTq_sb[:, 2 * j:2 * j + 2, 0:512],
                                         start=(j == 0), stop=(j == KH // 2 - 1),
                                         perf_mode=PM.DoubleRow)
                        nc.tensor.matmul(ps[:, 512:BL],
                                         lhsT=w2row[:, 2 * j:2 * j + 2, :],
                                         rhs=hTq_sb[:, 2 * j:2 * j + 2, 512:BL],
                                         start=(j == 0), stop=(j == KH // 2 - 1),
                                         perf_mode=PM.DoubleRow)
                    if with_b2:
                        S.activation(uT_sb[:, mh, :], ps, AF.Sigmoid,
                                     bias=b2_sb[:, mh:mh + 1], scale=inv_s2)
                    else:
                        S.activation(uT_sb[:, mh, :], ps, AF.Sigmoid,
                                     scale=inv_s2)
                    uu = scr.tile([P, BL], bf16, name="uu", tag="hh")
                    S.activation(uu, uT_sb[:, mh, :], AF.Square)
                    hu = scr.tile([P, BL], bf16, name="hu", tag="hh")
                    V.tensor_mul(hu, hT_sb[:, mh, :], uT_sb[:, mh, :])
                    if mh == 0:
                        V.tensor_copy(y2a, uu)
                        V.tensor_copy(xya, hu)
                    else:
                        V.tensor_add(y2a, y2a, uu)
                        V.tensor_add(xya, xya, hu)
                # warm-keeper group 1: bridges the PE from mm2's last matmul
                # across the stats-accumulation drain (a >3.4us idle would
                # HAM-throttle the clock for the MMo matmuls).  A real
                # accumulation group so DCE cannot drop it.
                for i in range(14):
                    nc.tensor.matmul(dum_ps, lhsT=ident_sb,
                                     rhs=hT_sb[:, i, 0:512],
                                     start=(i == 0), stop=(i == 13),
                                     skip_group_check=True)
                for blk in range(NB):
                    nc.tensor.matmul(stat_ps[:, 8 + blk:9 + blk],
                                     lhsT=y2a[:, blk * P:(blk + 1) * P],
                                     rhs=onesb, start=True, stop=True,
                                     skip_group_check=True)
                    nc.tensor.matmul(stat_ps[:, 16 + blk:17 + blk],
                                     lhsT=xya[:, blk * P:(blk + 1) * P],
                                     rhs=onesb, start=True, stop=True,
                                     skip_group_check=True)

            # ---------- per-row scalar chain, batch-major [128, 8] ----------
            alpha_b = abp.tile([P, BL], bf16, name="alpha_b")
            beta_b = abp.tile([P, BL], bf16, name="beta_b")
            ab_d = dram.tile([2, BL], bf16, name="ab_d")

            with nc.named_scope("scalars"):
                stats_sb = sc("stats_sb", (P, 24))
                V.tensor_copy(stats_sb, stat_ps)
                x2 = stats_sb[:, 0:8]
                y2 = stats_sb[:, 8:16]
                xy = stats_sb[:, 16:24]
                w = sc("w")
                V.scalar_tensor_tensor(out=w, in0=xy, scalar=-2.0, in1=y2,
                                       op0=ALU.mult, op1=ALU.add)
                A1 = sc("A1")
                V.tensor_scalar(out=A1, in0=w, scalar1=c_b, scalar2=1.0,
                                op0=ALU.mult, op1=ALU.add)
                A2 = sc("A2")
                V.tensor_scalar(out=A2, in0=x2, scalar1=negc_b, scalar2=1.0,
                                op0=ALU.mult, op1=ALU.add)
                p1 = sc("p1")
                V.tensor_mul(p1, x2, y2)
                den = sc("den")
                V.tensor_scalar(out=den, in0=p1, scalar1=c2_b, scalar2=1.0,
                                op0=ALU.mult, op1=ALU.add)
                V.scalar_tensor_tensor(out=den, in0=xy, scalar=neg2c_b, in1=den,
                                       op0=ALU.mult, op1=ALU.add)
                V.tensor_scalar_add(out=den, in0=den, scalar1=EPS)
                D = sc("D")
                V.reciprocal(D, den)
                # ||a||^2 = D^2 (A1^2 x2 - 2 A1 A2 xy + A2^2 y2)
                t1 = sc("t1")
                V.tensor_mul(t1, A1, A1)
                V.tensor_mul(t1, t1, x2)
                t2 = sc("t2")
                V.tensor_mul(t2, A1, A2)
                V.tensor_mul(t2, t2, xy)
                t3 = sc("t3")
                V.tensor_mul(t3, A2, A2)
                V.tensor_mul(t3, t3, y2)
                na2 = sc("na2")
                V.scalar_tensor_tensor(out=na2, in0=t2, scalar=-2.0, in1=t1,
                                       op0=ALU.mult, op1=ALU.add)
                V.tensor_add(na2, na2, t3)
                dsq = sc("dsq")
                V.tensor_mul(dsq, D, D)
                V.tensor_mul(na2, na2, dsq)
                # q = sqrt(c * na2) with one Newton step (ACT sqrt is low precision)
                q2 = sc("q2")
                V.tensor_scalar(out=q2, in0=na2, scalar1=c_b, scalar2=None,
                                op0=ALU.mult)
                q0 = sc("q0")
                S.activation(q0, q2, AF.Sqrt)
                V.tensor_scalar_max(out=q0, in0=q0, scalar1=1e-20)
                r0 = sc("r0")
                V.reciprocal(r0, q0)
                q = sc("q")
                V.tensor_mul(q, q2, r0)
                V.tensor_add(q, q, q0)
                V.tensor_scalar_mul(out=q, in0=q, scalar1=0.5)
                arg = sc("arg")
                V.tensor_scalar_min(out=arg, in0=q, scalar1=1.0 - 1e-5)
                # artanh(arg) = 0.5 ln((1+arg)/(1-arg)); t = tanh(T*artanh)/q
                opp = sc("opp")
                V.tensor_scalar(out=opp, in0=arg, scalar1=-1.0, scalar2=1.0,
                                op0=ALU.mult, op1=ALU.add)
                opn = sc("opn")
                V.tensor_scalar_add(out=opn, in0=arg, scalar1=1.0)
                rr = sc("rr")
                V.reciprocal(rr, opp)
                rat = sc("rat")
                V.tensor_mul(rat, opn, rr)
                lg = sc("lg")
                S.activation(lg, rat, AF.Ln)
                th = sc("th")
                S.activation(th, lg, AF.Tanh, scale=T_CONST * 0.5)
                rq = sc("rq")
                V.reciprocal(rq, q)
                tm = sc("tm")
                V.tensor_mul(tm, th, rq)
                # <h,a> = D (A2 xy - A1 x2)
                s1_ = sc("s1_")
                V.tensor_mul(s1_, A1, x2)
                s2_ = sc("s2_")
                V.tensor_mul(s2_, A2, xy)
                ha = sc("ha")
                V.tensor_sub(ha, s2_, s1_)
                V.tensor_mul(ha, ha, D)
                hm = sc("hm")
                V.tensor_mul(hm, tm, ha)
                tsq = sc("tsq")
                V.tensor_mul(tsq, tm, tm)
                m2 = sc("m2")
                V.tensor_mul(m2, tsq, na2)
                w2s = sc("w2s")
                V.scalar_tensor_tensor(out=w2s, in0=hm, scalar=2.0, in1=m2,
                                       op0=ALU.mult, op1=ALU.add)
                B1 = sc("B1")
                V.tensor_scalar(out=B1, in0=w2s, scalar1=c_b, scalar2=1.0,
                                op0=ALU.mult, op1=ALU.add)
                p2 = sc("p2")
                V.tensor_mul(p2, x2, m2)
                den2 = sc("den2")
                V.tensor_scalar(out=den2, in0=p2, scalar1=c2_b, scalar2=1.0,
                                op0=ALU.mult, op1=ALU.add)
                V.scalar_tensor_tensor(out=den2, in0=hm, scalar=twoc_b, in1=den2,
                                       op0=ALU.mult, op1=ALU.add)
                V.tensor_scalar_add(out=den2, in0=den2, scalar1=EPS)
                D2 = sc("D2")
                V.reciprocal(D2, den2)
                g = sc("g")
                V.tensor_mul(g, A2, tm)
                V.tensor_mul(g, g, D)
                w3 = sc("w3")
                V.tensor_mul(w3, g, A1)
                V.tensor_sub(w3, B1, w3)
                # alpha -> cols 0:8, beta -> cols 8:16 of one bf16 tile; a PE
                # transpose then yields batch-linear rows for a fast DMA
                # bounce (8 contiguous 256B descriptors instead of a 2-byte
                # element scatter).
                ab_bm = sc("ab_bm", (P, 16), bf16)
                V.tensor_mul(ab_bm[:, 0:8], w3, D2)
                w4 = sc("w4")
                V.tensor_mul(w4, g, A2)
                V.tensor_mul(ab_bm[:, 8:16], w4, D2)
                # warm-keeper group 2: keep the PE busy through the scalar
                # chain so HAM doesn't re-throttle the clock before the MMo
                # matmuls (idle > ~3.4us drops the PE to 1.2 GHz)
                for i in range(22):
                    nc.tensor.matmul(dum_ps, lhsT=ident_sb,
                                     rhs=hT_sb[:, 2 + i, 0:512],
                                     start=(i == 0), stop=(i == 21),
                                     skip_group_check=True)
                nc.tensor.transpose(abT_ps, ab_bm, ident_sb)
                for i in range(8):
                    nc.tensor.matmul(dum_ps, lhsT=ident_sb,
                                     rhs=hT_sb[:, 24 + i, 512:BL],
                                     start=(i == 0), stop=(i == 7),
                                     skip_group_check=True)
                # anchor the warm-keeper matmuls against dead-code elimination
                dum_sb = sc("dum_sb", (1, 1))
                V.tensor_copy(dum_sb, dum_ps[0:1, 0:1])
                dum_d = dram.tile([1, 1], f32, name="dum_d")
                nc.sync.dma_start(out=dum_d, in_=dum_sb)
                abT_sb = sc("abT_sb", (16, P), bf16)
                V.tensor_copy(abT_sb, abT_ps)
                nc.sync.dma_start(out=ab_d[0, :].rearrange("(j b) -> j b", j=8),
                                  in_=abT_sb[0:8, :])
                nc.sync.dma_start(out=ab_d[1, :].rearrange("(j b) -> j b", j=8),
                                  in_=abT_sb[8:16, :])
                for hsl in (slice(0, 512), slice(512, BL)):
                    nc.gpsimd.dma_start(out=alpha_b[:, hsl],
                                        in_=ab_d[0:1, hsl].to_broadcast([P, 512]))
                    nc.scalar.dma_start(out=beta_b[:, hsl],
                                        in_=ab_d[1:2, hsl].to_broadcast([P, 512]))

        # psum pools (mm, stp) released here

        # ---------- z = alpha*h + beta*u (overwrites uT in place),
        # then out = z @ Wo.  Processed in two batch-column halves so the
        # MMo matmuls of half 0 overlap the DVE z-combine of half 1.
        with ExitStack() as ph2:
            mmo = ph2.enter_context(tc.tile_pool(name="mmo", bufs=8,
                                                 space="PSUM"))
            for bg in range(2):
                csl = slice(bg * 512, (bg + 1) * 512)
                with nc.named_scope(f"zcomb{bg}"):
                    for kh in range(KH):
                        t1z = zzp.tile([P, 512], bf16, name="t1z", tag="zz")
                        V.tensor_mul(t1z, hT_sb[:, kh, csl], alpha_b[:, csl])
                        t2z = zzp.tile([P, 512], bf16, name="t2z", tag="zz")
                        V.tensor_mul(t2z, uT_sb[:, kh, csl], beta_b[:, csl])
                        V.tensor_add(uT_sb[:, kh, csl], t1z, t2z)
                with nc.named_scope(f"mmo{bg}"):
                    pso = [mmo.tile([P, 500], f32, name=f"pso{bg}_{i}",
                                    tag="mmo") for i in range(8)]
                    for kh in range(KH):
                        wot = wop.tile([P, OUT], bf16, name="wot", tag="wo")
                        nc.sync.dma_start(out=wot, in_=wo_d[kh])
                        for i in range(4):
                            b = bg * 4 + i
                            for och in range(2):
                                nc.tensor.matmul(
                                    pso[i * 2 + och],
                                    lhsT=uT_sb[:, kh, b * P:(b + 1) * P],
                                    rhs=wot[:, och * 500:(och + 1) * 500],
                                    start=(kh == 0), stop=(kh == KH - 1))
                    for i in range(4):
                        b = bg * 4 + i
                        for och in range(2):
                            ob = outp.tile([P, 500], f32, name="ob", tag="ob")
                            if och == 0:
                                S.copy(ob, pso[i * 2])
                            else:
                                V.tensor_copy(ob, pso[i * 2 + 1])
                            nc.sync.dma_start(
                                out=out_d[b * P:(b + 1) * P,
                                          och * 500:(och + 1) * 500],
                                in_=ob)

    nc.compile()
    return nc


def _get_nc(with_b1, with_b2):
    for k, v in _nc_cache:
        if k == (with_b1, with_b2):
            return v
    nc = _build(with_b1, with_b2)
    _nc_cache.append(((with_b1, with_b2), nc))
    return nc


def kernel(x, W1, b1, W2, b2, Wo, bo, cp_w1, cp_b1, cp_w2, cp_b2,
           _trace=False, _tmpdir=None):
    x = np.asarray(x, dtype=np.float32)
    with_b1 = bool(np.any(b1))
    with_b2 = bool(np.any(b2))
    nc = _get_nc(with_b1, with_b2)

    # w1h[mh, g, p, kk, q] = W1[(g*12+kk)*128+p, mh*128+q]
    w1_t = np.ascontiguousarray(
        np.asarray(W1, np.float32).reshape(KI, P, KH, P).transpose(2, 1, 0, 3)
        .reshape(KH, P, 2, KI // 2, P).transpose(0, 2, 1, 3, 4)
    ).astype(BF)
    # w2r[mh, p, kh, q] = W2[kh*128+p, mh*128+q], fp8 e4m3 scaled x1024
    w2_t = np.ascontiguousarray(
        np.clip(np.asarray(W2, np.float32) * W2S, -E4MAX, E4MAX)
        .reshape(KH, P, KH, P).transpose(2, 1, 0, 3)
    ).astype(E4)
    wo_t = np.asarray(Wo, np.float32).reshape(KH, P, OUT).astype(BF)
    cpw1_t = np.ascontiguousarray(
        np.asarray(cp_w1, np.float32).T.reshape(KI, P, 16)).astype(BF)
    cpw2_t = np.asarray(cp_w2, np.float32).reshape(1, 16).T.astype(BF)
    cpw2_t = np.ascontiguousarray(cpw2_t)
    cpb1_t = np.asarray(cp_b1, np.float32).reshape(16, 1)
    cpb2_t = np.asarray(cp_b2, np.float32).reshape(1, 1)
    b1_t = np.ascontiguousarray(np.asarray(b1, np.float32).reshape(KH, P).T)
    b2_t = np.ascontiguousarray(np.asarray(b2, np.float32).reshape(KH, P).T)

    in_maps = []
    for c in range(N_CORES):
        shard = x[c * BL:(c + 1) * BL]
        xT = np.ascontiguousarray(shard.T).reshape(KI, P, BL).astype(BF)
        m = {"xT": xT, "w1": w1_t, "w2": w2_t, "wo": wo_t,
             "cpw1": cpw1_t, "cpw2": cpw2_t, "cpb1": cpb1_t, "cpb2": cpb2_t,
             "ident": _IDENT}
        if with_b1:
            m["b1"] = b1_t
        if with_b2:
            m["b2"] = b2_t
        in_maps.append(m)

    kw = {}
    if _trace:
        kw = dict(trace=True, tmpdir=_tmpdir or tempfile.mkdtemp(prefix="cdk_"))
    res = run_bass_kernel_spmd(nc, in_maps, list(range(N_CORES)), **kw)

    out = np.concatenate([res.results[c]["out"] for c in range(N_CORES)], axis=0)
    bo = np.asarray(bo, np.float32)
    if np.any(bo):
        out = out + bo
    if _trace:
        kernel._last_result = res
    return out


# revision 14
# speedup vs baseline: 1.3132x; 1.0321x over previous
"""Trainium2 Bass kernel for nn_ConservativeDynamicCurvatureMLP.

Data-parallel over 8 NeuronCores: the batch (8192) is sharded into 8
local shards of 1024 rows; all weights are replicated. The curvature
scalar c_avg couples the shards through a global mean, handled with a
single-scalar AllReduce.

Math (reference):
    h = tanh(x @ W1 + b1)
    u = sigmoid(h @ W2 + b2)
    c = clip(mean(MIN_C + (MAX_C-MIN_C) * sigmoid(relu(x@cp_w1.T+cp_b1)@cp_w2.T+cp_b2)), MIN_C, MAX_C)
    z = poincare_ball_layer(h, u, c, T)
    out = z @ Wo + bo

The poincare layer collapses algebraically to z = alpha(row)*h + beta(row)*u
where alpha/beta are scalar functions of the row statistics
x2=||h||^2, y2=||u||^2, xy=<h,u> and c.  The NaN fallback is measure-zero
and omitted.

Perf notes (measured on hw):
  - With 8 cores active the PE is power-throttled to ~2.0 GHz (HAM state
    31), so bf16 N=512 matmuls pace at ~263 ns.  The only lever is fewer
    PE cycles: MM2 (h@W2, the largest matmul) runs as fp8-e4m3 DoubleRow
    (256-deep contraction per pass, 2x FLOP rate, measured 216 ns/MM at
    2.4 GHz in isolation).  W2 is host-converted to e4m3 * 1024 (clipped
    to +-240, the TRN e4m3 max); the 1/1024 folds into the sigmoid's
    activation scale.  h is cast bf16->fp8 per row-tile on the DVE.
    MM1 / MMo stay bf16: fp8 there pushes rel-err too close to the 2e-2
    gate (simulated 1.96e-2 vs 1.26e-2 for MM2-only).
  - Row statistics go batch-major directly via tiny stationary-squares
    matmuls (lhsT = accumulated squares block, rhs = ones) instead of a
    feature-major ones-reduction + DRAM bounce, shortening the serial
    stats -> alpha/beta tail.  The per-row scalar chain runs once on
    [128, 8] tiles reading stats straight from SBUF.
"""

import tempfile
from contextlib import ExitStack

import numpy as np
import ml_dtypes

import concourse.bass as bass
import concourse.bacc as bacc
import concourse.mybir as mybir
import concourse.tile as tile
from concourse.bass_utils import run_bass_kernel_spmd

P = 128
N_CORES = 8
B_FULL = 8192
BL = B_FULL // N_CORES          # 1024 rows per core
IN = 3072
HID = 4096
OUT = 1000
KI = IN // P                    # 24
KH = HID // P                   # 32
NB = BL // P                    # 8 batch tiles
MIN_C = 0.001 * 0.5
MAX_C = 0.001 * 2.0
T_CONST = 0.7
EPS = 1e-7
W2S = 1024.0                    # host-side fp8 scale for W2
E4MAX = 240.0                   # TRN e4m3 saturation

dt = mybir.dt
AF = mybir.ActivationFunctionType
ALU = mybir.AluOpType
PM = mybir.MatmulPerfMode
BF = ml_dtypes.bfloat16
E4 = ml_dtypes.float8_e4m3

_nc_cache = []
_IDENT = np.eye(P, dtype=ml_dtypes.bfloat16)


def _build(with_b1, with_b2):
    nc = bacc.Bacc("TRN2", target_bir_lowering=False, debug=False,
                   num_devices=N_CORES)

    xT_d = nc.dram_tensor("xT", [KI, P, BL], dt.bfloat16, kind="ExternalInput")
    # W1 rows split in two halves for finer DMA/SBUF: w1h[mh, g, p, kk, q] =
    # W1[(g*12+kk)*128+p, mh*128+q]
    w1_d = nc.dram_tensor("w1", [KH, 2, P, KI // 2, P], dt.bfloat16,
                          kind="ExternalInput")
    # W2 in fp8 (x1024): w2r[mh, p, kh, q] = W2[kh*128+p, mh*128+q]
    w2_d = nc.dram_tensor("w2", [KH, P, KH, P], dt.float8e4, kind="ExternalInput")
    wo_d = nc.dram_tensor("wo", [KH, P, OUT], dt.bfloat16, kind="ExternalInput")
    cpw1_d = nc.dram_tensor("cpw1", [KI, P, 16], dt.bfloat16, kind="ExternalInput")
    cpw2_d = nc.dram_tensor("cpw2", [16, 1], dt.bfloat16, kind="ExternalInput")
    cpb1_d = nc.dram_tensor("cpb1", [16, 1], dt.float32, kind="ExternalInput")
    cpb2_d = nc.dram_tensor("cpb2", [1, 1], dt.float32, kind="ExternalInput")
    ident_d = nc.dram_tensor("ident", [P, P], dt.bfloat16, kind="ExternalInput")
    b1_d = nc.dram_tensor("b1", [P, KH], dt.float32, kind="ExternalInput") if with_b1 else None
    b2_d = nc.dram_tensor("b2", [P, KH], dt.float32, kind="ExternalInput") if with_b2 else None
    out_d = nc.dram_tensor("out", [BL, OUT], dt.float32, kind="ExternalOutput")

    f32 = dt.float32
    bf16 = dt.bfloat16
    fp8 = dt.float8e4

    with tile.TileContext(nc) as tc, ExitStack() as ctx:
        const = ctx.enter_context(tc.tile_pool(name="const", bufs=1))
        big = ctx.enter_context(tc.tile_pool(name="big", bufs=1))
        htp = ctx.enter_context(tc.tile_pool(name="htp", bufs=1))
        hqp = ctx.enter_context(tc.tile_pool(name="hqp", bufs=1))
        wp = ctx.enter_context(tc.tile_pool(name="wp", bufs=2))
        wop = ctx.enter_context(tc.tile_pool(name="wop", bufs=3))
        scr = ctx.enter_context(tc.tile_pool(name="scr", bufs=2))
        zzp = ctx.enter_context(tc.tile_pool(name="zzp", bufs=4))
        sacc = ctx.enter_context(tc.tile_pool(name="sacc", bufs=1))
        abp = ctx.enter_context(tc.tile_pool(name="abp", bufs=1))
        scal = ctx.enter_context(tc.tile_pool(name="scal", bufs=1))
        outp = ctx.enter_context(tc.tile_pool(name="outp", bufs=2))
        cpp = ctx.enter_context(tc.tile_pool(name="cpp", bufs=1))
        dram = ctx.enter_context(tc.tile_pool(name="dram", bufs=1, space="DRAM"))

        V = nc.vector
        S = nc.scalar

        def sc(name, shape=(P, 8), dtype=f32):
            return scal.tile(list(shape), dtype, name=name, tag=name)

        # ---------- persistent activations (feature-major) ----------
        onesb = const.tile([P, 1], bf16, name="onesb")
        nc.vector.memset(onesb, 1.0)
        xT_sb = big.tile([P, KI, BL], bf16, name="xT_sb", tag="big",
                         padded_shape=[P, KH, BL])
        # first weight half-row issues on the sync queue before anything else;
        # xT streams on the gpsimd + scalar queues in parallel (the load is
        # HBM-bandwidth-bound, ~35us for 6.3MB with all 8 cores pulling)
        w1row0 = wp.tile([P, KI // 2, P], bf16, name="w1row", tag="w1")
        nc.sync.dma_start(out=w1row0, in_=w1_d[0, 0])
        xq = [nc.gpsimd, nc.scalar]
        for idx, (a, b) in enumerate(((0, 2), (2, 4), (4, 7), (7, 10),
                                      (10, 13), (13, 16), (16, 20), (20, 24))):
            xq[idx % 2].dma_start(
                out=xT_sb[:, a:b, :],
                in_=xT_d[a:b].rearrange("k p b -> p k b"))
        ident_sb = const.tile([P, P], bf16, name="ident_sb")
        nc.sync.dma_start(out=ident_sb, in_=ident_d[:, :])
        hT_sb = htp.tile([P, KH, BL], bf16, name="hT_sb")
        hTq_sb = hqp.tile([P, KH, BL], fp8, name="hTq_sb")
        if with_b1:
            b1_sb = const.tile([P, KH], f32, name="b1_sb")
            nc.sync.dma_start(out=b1_sb, in_=b1_d[:, :])
        if with_b2:
            b2_sb = const.tile([P, KH], f32, name="b2_sb")
            nc.sync.dma_start(out=b2_sb, in_=b2_d[:, :])

        with ExitStack() as ph1:
            mm = ph1.enter_context(tc.tile_pool(name="mm", bufs=3, space="PSUM"))
            stp = ph1.enter_context(tc.tile_pool(name="stp", bufs=1, space="PSUM"))
            # one bank shared by the batch-major stats (cols 0:8 x2,
            # 8:16 y2, 16:24 xy) and the HAM warm-keeper matmul target
            # (cols 128:512); plus the a/b transpose target bank
            stat_ps = stp.tile([P, 512], f32, name="stat_ps")
            dum_ps = stat_ps[:, 128:512]
            abT_ps = stp.tile([16, P], bf16, name="abT_ps")

            # ---------- MM1: hT = tanh(W1.T @ xT) (bf16) ----------
            x2a = sacc.tile([P, BL], bf16, name="x2a")
            y2a = sacc.tile([P, BL], bf16, name="y2a")
            xya = sacc.tile([P, BL], bf16, name="xya")
            with nc.named_scope("mm1"):
                for mh in range(KH):
                    ps = mm.tile([P, BL], f32, name="ps", tag="mm")
                    if mh == 0:
                        w1a = w1row0
                    else:
                        w1a = wp.tile([P, KI // 2, P], bf16, name="w1row",
                                      tag="w1")
                        nc.sync.dma_start(out=w1a, in_=w1_d[mh, 0])
                    w1b = wp.tile([P, KI // 2, P], bf16, name="w1row", tag="w1")
                    nc.sync.dma_start(out=w1b, in_=w1_d[mh, 1])
                    for ki in range(KI):
                        wrow = w1a if ki < 12 else w1b
                        kk = ki % 12
                        nc.tensor.matmul(ps[:, 0:512], lhsT=wrow[:, kk, :],
                                         rhs=xT_sb[:, ki, 0:512],
                                         start=(ki == 0), stop=(ki == KI - 1))
                        nc.tensor.matmul(ps[:, 512:BL], lhsT=wrow[:, kk, :],
                                         rhs=xT_sb[:, ki, 512:BL],
                                         start=(ki == 0), stop=(ki == KI - 1))
                    if with_b1:
                        S.activation(hT_sb[:, mh, :], ps, AF.Tanh,
                                     bias=b1_sb[:, mh:mh + 1])
                    else:
                        S.activation(hT_sb[:, mh, :], ps, AF.Tanh)
                    # fp8 copy for MM2's DoubleRow rhs
                    V.tensor_copy(hTq_sb[:, mh, :], hT_sb[:, mh, :])
                    hh = scr.tile([P, BL], bf16, name="hh", tag="hh")
                    S.activation(hh, hT_sb[:, mh, :], AF.Square)
                    if mh == 0:
                        V.tensor_copy(x2a, hh)
                    else:
                        V.tensor_add(x2a, x2a, hh)
                # x2 -> batch-major: out[b,0] = sum_p x2a[p, blk*128+b]
                for blk in range(NB):
                    nc.tensor.matmul(stat_ps[:, blk:blk + 1],
                                     lhsT=x2a[:, blk * P:(blk + 1) * P],
                                     rhs=onesb, start=True, stop=True,
                                     skip_group_check=True)

            # ---------- curvature predictor (xT still resident; the
            # AllReduce hides under MM1/MM2) ----------
            with nc.named_scope("cp"):
                cpw1_sb = const.tile([P, KI, 16], bf16, name="cpw1_sb")
                nc.sync.dma_start(out=cpw1_sb,
                                  in_=cpw1_d.rearrange("k p q -> p k q"))
                cpw2_sb = const.tile([16, 1], bf16, name="cpw2_sb")
                nc.sync.dma_start(out=cpw2_sb, in_=cpw2_d[:, :])
                cpb1_sb = const.tile([16, 1], f32, name="cpb1_sb")
                nc.sync.dma_start(out=cpb1_sb, in_=cpb1_d[:, :])
                cpb2_sb = const.tile([1, 1], f32, name="cpb2_sb")
                nc.sync.dma_start(out=cpb2_sb, in_=cpb2_d[:, :])
                cph_sb = cpp.tile([16, BL], bf16, name="cph_sb")
                for ch in range(2):
                    cps = mm.tile([16, 512], f32, name="cps", tag="mm")
                    for ki in range(KI):
                        nc.tensor.matmul(
                            cps, lhsT=cpw1_sb[:, ki, :],
                            rhs=xT_sb[:, ki, ch * 512:(ch + 1) * 512],
                            start=(ki == 0), stop=(ki == KI - 1))
                    S.activation(cph_sb[:, ch * 512:(ch + 1) * 512], cps,
                                 AF.Relu, bias=cpb1_sb)
                sparts = []
                for ch in range(2):
                    c2p = mm.tile([1, 512], f32, name="c2p", tag="mm")
                    nc.tensor.matmul(c2p, lhsT=cpw2_sb,
                                     rhs=cph_sb[:16, ch * 512:(ch + 1) * 512],
                                     start=True, stop=True)
                    cpw = cpp.tile([1, 512], bf16, name="cpw", tag="cpw")
                    spart = cpp.tile([1, 1], f32, name=f"spart{ch}",
                                     tag=f"spart{ch}")
                    S.activation(cpw, c2p, AF.Sigmoid, bias=cpb2_sb,
                                 accum_out=spart)
                    sparts.append(spart)
                s_loc = cpp.tile([1, 1], f32, name="s_loc")
                V.tensor_add(s_loc, sparts[0], sparts[1])
                cin = dram.tile([1, 1], f32, name="cin")
                cout = dram.tile([1, 1], f32, name="cout")
                nc.sync.dma_start(out=cin, in_=s_loc)
                nc.gpsimd.collective_compute(
                    "AllReduce", ALU.add,
                    replica_groups=[list(range(N_CORES))],
                    ins=[cin.opt()], outs=[cout.opt()])
                s_b = sc("s_b", (P, 1))
                nc.gpsimd.dma_start(out=s_b, in_=cout.to_broadcast([P, 1]))
                # c = clip(MIN_C + (MAX_C-MIN_C)*mean(c_pred))
                c_b = sc("c_b", (P, 1))
                V.tensor_scalar(out=c_b, in0=s_b,
                                scalar1=(MAX_C - MIN_C) / B_FULL,
                                scalar2=MIN_C, op0=ALU.mult, op1=ALU.add)
                V.tensor_scalar_min(out=c_b, in0=c_b, scalar1=MAX_C)
                V.tensor_scalar_max(out=c_b, in0=c_b, scalar1=MIN_C)
                negc_b = sc("negc_b", (P, 1))
                V.tensor_scalar_mul(out=negc_b, in0=c_b, scalar1=-1.0)
                twoc_b = sc("twoc_b", (P, 1))
                V.tensor_scalar_mul(out=twoc_b, in0=c_b, scalar1=2.0)
                neg2c_b = sc("neg2c_b", (P, 1))
                V.tensor_scalar_mul(out=neg2c_b, in0=c_b, scalar1=-2.0)
                c2_b = sc("c2_b", (P, 1))
                V.tensor_mul(c2_b, c_b, c_b)

            # ---------- MM2: uT = sigmoid(W2.T @ hT) fp8 DoubleRow ----------
            uT_sb = big.tile([P, KH, BL], bf16, name="uT_sb", tag="big")
            inv_s2 = 1.0 / W2S
            with nc.named_scope("mm2"):
                for mh in range(KH):
                    ps = mm.tile([P, BL], f32, name="ps", tag="mm")
                    w2row = wp.tile([P, KH, P], fp8, name="w2row", tag="w2")
                    nc.sync.dma_start(out=w2row, in_=w2_d[mh])
                    for j in range(KH // 2):
                        nc.tensor.matmul(ps[:, 0:512],
                                         lhsT=w2row[:, 2 * j:2 * j + 2, :],
                                         rhs=hTq_sb[:, 2 * j:2 * j + 2, 0:512],
                                         start=(j == 0), stop=(j == KH // 2 - 1),
                                         perf_mode=PM.DoubleRow)
                        nc.tensor.matmul(ps[:, 512:BL],
                                         lhsT=w2row[:, 2 * j:2 * j + 2, :],
                                         rhs=hTq_sb[:, 2 * j:2 * j + 2, 512:BL],
                                         start=(j == 0), stop=(j == KH // 2 - 1),
                                         perf_mode=PM.DoubleRow)
                    if with_b2:
                        S.activation(uT_sb[:, mh, :], ps, AF.Sigmoid,
                                     bias=b2_sb[:, mh:mh + 1], scale=inv_s2)
                    else:
                        S.activation(uT_sb[:, mh, :], ps, AF.Sigmoid,
                                     scale=inv_s2)
                    uu = scr.tile([P, BL], bf16, name="uu", tag="hh")
                    S.activation(uu, uT_sb[:, mh, :], AF.Square)
                    hu = scr.tile([P, BL], bf16, name="hu", tag="hh")
                    V.tensor_mul(hu, hT_sb[:, mh, :], uT_sb[:, mh, :])
                    if mh == 0:
                        V.tensor_copy(y2a, uu)
                        V.tensor_copy(xya, hu)
                    else:
                        V.tensor_add(y2a, y2a, uu)
                        V.tensor_add(xya, xya, hu)
                # warm-keeper group 1: bridges the PE from mm2's last matmul
                # across the stats-accumulation drain (a >3.4us idle would
                # HAM-throttle the clock for the MMo matmuls).  A real
                # accumulation group so DCE cannot drop it.
                for i in range(14):
                    nc.tensor.matmul(dum_ps, lhsT=ident_sb,
                                     rhs=hT_sb[:, i, 0:384],
                                     start=(i == 0), stop=(i == 13),
                                     skip_group_check=True)
                for blk in range(NB):
                    nc.tensor.matmul(stat_ps[:, 8 + blk:9 + blk],
                                     lhsT=y2a[:, blk * P:(blk + 1) * P],
                                     rhs=onesb, start=True, stop=True,
                                     skip_group_check=True)
                    nc.tensor.matmul(stat_ps[:, 16 + blk:17 + blk],
                                     lhsT=xya[:, blk * P:(blk + 1) * P],
                                     rhs=onesb, start=True, stop=True,
                                     skip_group_check=True)

            # ---------- per-row scalar chain, batch-major [128, 8] ----------
            alpha_b = abp.tile([P, BL], bf16, name="alpha_b")
            beta_b = abp.tile([P, BL], bf16, name="beta_b")
            ab_d = dram.tile([2, BL], bf16, name="ab_d")

            with nc.named_scope("scalars"):
                stats_sb = sc("stats_sb", (P, 24))
                V.tensor_copy(stats_sb, stat_ps[:, 0:24])
                x2 = stats_sb[:, 0:8]
                y2 = stats_sb[:, 8:16]
                xy = stats_sb[:, 16:24]
                w = sc("w")
                V.scalar_tensor_tensor(out=w, in0=xy, scalar=-2.0, in1=y2,
                                       op0=ALU.mult, op1=ALU.add)
                A1 = sc("A1")
                V.tensor_scalar(out=A1, in0=w, scalar1=c_b, scalar2=1.0,
                                op0=ALU.mult, op1=ALU.add)
                A2 = sc("A2")
                V.tensor_scalar(out=A2, in0=x2, scalar1=negc_b, scalar2=1.0,
                                op0=ALU.mult, op1=ALU.add)
                p1 = sc("p1")
                V.tensor_mul(p1, x2, y2)
                den = sc("den")
                V.tensor_scalar(out=den, in0=p1, scalar1=c2_b, scalar2=1.0,
                                op0=ALU.mult, op1=ALU.add)
                V.scalar_tensor_tensor(out=den, in0=xy, scalar=neg2c_b, in1=den,
                                       op0=ALU.mult, op1=ALU.add)
                V.tensor_scalar_add(out=den, in0=den, scalar1=EPS)
                D = sc("D")
                V.reciprocal(D, den)
                # ||a||^2 = D^2 (A1^2 x2 - 2 A1 A2 xy + A2^2 y2)
                t1 = sc("t1")
                V.tensor_mul(t1, A1, A1)
                V.tensor_mul(t1, t1, x2)
                t2 = sc("t2")
                V.tensor_mul(t2, A1, A2)
                V.tensor_mul(t2, t2, xy)
                t3 = sc("t3")
                V.tensor_mul(t3, A2, A2)
                V.tensor_mul(t3, t3, y2)
                na2 = sc("na2")
                V.scalar_tensor_tensor(out=na2, in0=t2, scalar=-2.0, in1=t1,
                                       op0=ALU.mult, op1=ALU.add)
                V.tensor_add(na2, na2, t3)
                dsq = sc("dsq")
                V.tensor_mul(dsq, D, D)
                V.tensor_mul(na2, na2, dsq)
                # q = sqrt(c * na2) with one Newton step (ACT sqrt is low precision)
                q2 = sc("q2")
                V.tensor_scalar(out=q2, in0=na2, scalar1=c_b, scalar2=None,
                                op0=ALU.mult)
                q0 = sc("q0")
                S.activation(q0, q2, AF.Sqrt)
                V.tensor_scalar_max(out=q0, in0=q0, scalar1=1e-20)
                r0 = sc("r0")
                V.reciprocal(r0, q0)
                q = sc("q")
                V.tensor_mul(q, q2, r0)
                V.tensor_add(q, q, q0)
                V.tensor_scalar_mul(out=q, in0=q, scalar1=0.5)
                arg = sc("arg")
                V.tensor_scalar_min(out=arg, in0=q, scalar1=1.0 - 1e-5)
                # artanh(arg) = 0.5 ln((1+arg)/(1-arg)); t = tanh(T*artanh)/q
                opp = sc("opp")
                V.tensor_scalar(out=opp, in0=arg, scalar1=-1.0, scalar2=1.0,
                                op0=ALU.mult, op1=ALU.add)
                opn = sc("opn")
                V.tensor_scalar_add(out=opn, in0=arg, scalar1=1.0)
                rr = sc("rr")
                V.reciprocal(rr, opp)
                rat = sc("rat")
                V.tensor_mul(rat, opn, rr)
                lg = sc("lg")
                S.activation(lg, rat, AF.Ln)
                th = sc("th")
                S.activation(th, lg, AF.Tanh, scale=T_CONST * 0.5)
                rq = sc("rq")
                V.reciprocal(rq, q)
                tm = sc("tm")
                V.tensor_mul(tm, th, rq)
                # <h,a> = D (A2 xy - A1 x2)
                s1_ = sc("s1_")
                V.tensor_mul(s1_, A1, x2)
                s2_ = sc("s2_")
                V.tensor_mul(s2_, A2, xy)
                ha = sc("ha")
                V.tensor_sub(ha, s2_, s1_)
                V.tensor_mul(ha, ha, D)
                hm = sc("hm")
                V.tensor_mul(hm, tm, ha)
                tsq = sc("tsq")
                V.tensor_mul(tsq, tm, tm)
                m2 = sc("m2")
                V.tensor_mul(m2, tsq, na2)
                w2s = sc("w2s")
                V.scalar_tensor_tensor(out=w2s, in0=hm, scalar=2.0, in1=m2,
                                       op0=ALU.mult, op1=ALU.add)
                B1 = sc("B1")
                V.tensor_scalar(out=B1, in0=w2s, scalar1=c_b, scalar2=1.0,
                                op0=ALU.mult, op1=ALU.add)
                p2 = sc("p2")
                V.tensor_mul(p2, x2, m2)
                den2 = sc("den2")
                V.tensor_scalar(out=den2, in0=p2, scalar1=c2_b, scalar2=1.0,
                                op0=ALU.mult, op1=ALU.add)
                V.scalar_tensor_tensor(out=den2, in0=hm, scalar=twoc_b, in1=den2,
                                       op0=ALU.mult, op1=ALU.add)
                V.tensor_scalar_add(out=den2, in0=den2, scalar1=EPS)
                D2 = sc("D2")
                V.reciprocal(D2, den2)
                g = sc("g")
                V.tensor_mul(g, A2, tm)
                V.tensor_mul(g, g, D)
                w3 = sc("w3")
                V.tensor_mul(w3, g, A1)
                V.tensor_sub(w3, B1, w3)
                # alpha -> cols 0:8, beta -> cols 8:16 of one bf16 tile; a PE
                # transpose then yields batch-linear rows for a fast DMA
                # bounce (8 contiguous 256B descriptors instead of a 2-byte
                # element scatter).
                ab_bm = sc("ab_bm", (P, 16), bf16)
                V.tensor_mul(ab_bm[:, 0:8], w3, D2)
                w4 = sc("w4")
                V.tensor_mul(w4, g, A2)
                V.tensor_mul(ab_bm[:, 8:16], w4, D2)
                # warm-keeper group 2: keep the PE busy through the scalar
                # chain so HAM doesn't re-throttle the clock before the MMo
                # matmuls (idle > ~3.4us drops the PE to 1.2 GHz)
                for i in range(4):
                    nc.tensor.matmul(dum_ps, lhsT=ident_sb,
                                     rhs=hT_sb[:, 2 + i, 0:384],
                                     start=(i == 0), stop=(i == 3),
                                     skip_group_check=True)
                nc.tensor.transpose(abT_ps, ab_bm, ident_sb)
                for i in range(18):
                    nc.tensor.matmul(dum_ps, lhsT=ident_sb,
                                     rhs=hT_sb[:, 6 + i, 512:896],
                                     start=(i == 0), stop=(i == 17),
                                     skip_group_check=True)
                # anchor the warm-keeper matmuls against dead-code elimination
                dum_sb = sc("dum_sb", (1, 1))
                V.tensor_copy(dum_sb, dum_ps[0:1, 0:1])
                dum_d = dram.tile([1, 1], f32, name="dum_d")
                nc.sync.dma_start(out=dum_d, in_=dum_sb)
                abT_sb = sc("abT_sb", (16, P), bf16)
                V.tensor_copy(abT_sb, abT_ps)
                nc.sync.dma_start(out=ab_d[0, :].rearrange("(j b) -> j b", j=8),
                                  in_=abT_sb[0:8, :])
                nc.sync.dma_start(out=ab_d[1, :].rearrange("(j b) -> j b", j=8),
                                  in_=abT_sb[8:16, :])
                for hsl in (slice(0, 512), slice(512, BL)):
                    nc.gpsimd.dma_start(out=alpha_b[:, hsl],
                                        in_=ab_d[0:1, hsl].to_broadcast([P, 512]))
                    nc.scalar.dma_start(out=beta_b[:, hsl],
                                        in_=ab_d[1:2, hsl].to_broadcast([P, 512]))

        # psum pools (mm, stp) released here

        # ---------- z = alpha*h + beta*u (overwrites uT in place),
        # then out = z @ Wo.  Processed in two batch-column halves so the
        # MMo matmuls of half 0 overlap the DVE z-combine of half 1.
        with ExitStack() as ph2:
            mmo = ph2.enter_context(tc.tile_pool(name="mmo", bufs=8,
                                                 space="PSUM"))
            for bg in range(2):
                csl = slice(bg * 512, (bg + 1) * 512)
                with nc.named_scope(f"zcomb{bg}"):
                    for kh in range(KH):
                        t1z = zzp.tile([P, 512], bf16, name="t1z", tag="zz")
                        V.tensor_mul(t1z, hT_sb[:, kh, csl], alpha_b[:, csl])
                        t2z = zzp.tile([P, 512], bf16, name="t2z", tag="zz")
                        V.tensor_mul(t2z, uT_sb[:, kh, csl], beta_b[:, csl])
                        V.tensor_add(uT_sb[:, kh, csl], t1z, t2z)
                with nc.named_scope(f"mmo{bg}"):
                    pso = [mmo.tile([P, 500], f32, name=f"pso{bg}_{i}",
                                    tag="mmo") for i in range(8)]
                    for kh in range(KH):
                        wot = wop.tile([P, OUT], bf16, name="wot", tag="wo")
                        nc.sync.dma_start(out=wot, in_=wo_d[kh])
                        for i in range(4):
                            b = bg * 4 + i
                            for och in range(2):
                                nc.tensor.matmul(
                                    pso[i * 2 + och],
                                    lhsT=uT_sb[:, kh, b * P:(b + 1) * P],
                                    rhs=wot[:, och * 500:(och + 1) * 500],
                                    start=(kh == 0), stop=(kh == KH - 1))
                    for i in range(4):
                        b = bg * 4 + i
                        for och in range(2):
                            ob = outp.tile([P, 500], f32, name="ob", tag="ob")
                            if och == 0:
                                S.copy(ob, pso[i * 2])
                            else:
                                V.tensor_copy(ob, pso[i * 2 + 1])
                            nc.sync.dma_start(
                                out=out_d[b * P:(b + 1) * P,
                                          och * 500:(och + 1) * 500],
                                in_=ob)

    nc.compile()
    return nc


def _get_nc(with_b1, with_b2):
    for k, v in _nc_cache:
        if k == (with_b1, with_b2):
            return v
    nc = _build(with_b1, with_b2)
    _nc_cache.append(((with_b1, with_b2), nc))
    return nc


def kernel(x, W1, b1, W2, b2, Wo, bo, cp_w1, cp_b1, cp_w2, cp_b2,
           _trace=False, _tmpdir=None):
    x = np.asarray(x, dtype=np.float32)
    with_b1 = bool(np.any(b1))
    with_b2 = bool(np.any(b2))
    nc = _get_nc(with_b1, with_b2)

    # w1h[mh, g, p, kk, q] = W1[(g*12+kk)*128+p, mh*128+q]
    w1_t = np.ascontiguousarray(
        np.asarray(W1, np.float32).reshape(KI, P, KH, P).transpose(2, 1, 0, 3)
        .reshape(KH, P, 2, KI // 2, P).transpose(0, 2, 1, 3, 4)
    ).astype(BF)
    # w2r[mh, p, kh, q] = W2[kh*128+p, mh*128+q], fp8 e4m3 scaled x1024
    w2_t = np.ascontiguousarray(
        np.clip(np.asarray(W2, np.float32) * W2S, -E4MAX, E4MAX)
        .reshape(KH, P, KH, P).transpose(2, 1, 0, 3)
    ).astype(E4)
    wo_t = np.asarray(Wo, np.float32).reshape(KH, P, OUT).astype(BF)
    cpw1_t = np.ascontiguousarray(
        np.asarray(cp_w1, np.float32).T.reshape(KI, P, 16)).astype(BF)
    cpw2_t = np.asarray(cp_w2, np.float32).reshape(1, 16).T.astype(BF)
    cpw2_t = np.ascontiguousarray(cpw2_t)
    cpb1_t = np.asarray(cp_b1, np.float32).reshape(16, 1)
    cpb2_t = np.asarray(cp_b2, np.float32).reshape(1, 1)
    b1_t = np.ascontiguousarray(np.asarray(b1, np.float32).reshape(KH, P).T)
    b2_t = np.ascontiguousarray(np.asarray(b2, np.float32).reshape(KH, P).T)

    in_maps = []
    for c in range(N_CORES):
        shard = x[c * BL:(c + 1) * BL]
        xT = np.ascontiguousarray(shard.T).reshape(KI, P, BL).astype(BF)
        m = {"xT": xT, "w1": w1_t, "w2": w2_t, "wo": wo_t,
             "cpw1": cpw1_t, "cpw2": cpw2_t, "cpb1": cpb1_t, "cpb2": cpb2_t,
             "ident": _IDENT}
        if with_b1:
            m["b1"] = b1_t
        if with_b2:
            m["b2"] = b2_t
        in_maps.append(m)

    kw = {}
    if _trace:
        kw = dict(trace=True, tmpdir=_tmpdir or tempfile.mkdtemp(prefix="cdk_"))
    res = run_bass_kernel_spmd(nc, in_maps, list(range(N_CORES)), **kw)

    out = np.concatenate([res.results[c]["out"] for c in range(N_CORES)], axis=0)
    bo = np.asarray(bo, np.float32)
    if np.any(bo):
        out = out + bo
    if _trace:
        kernel._last_result = res
    return out


# revision 16
# speedup vs baseline: 1.3236x; 1.0079x over previous
"""Trainium2 Bass kernel for nn_ConservativeDynamicCurvatureMLP.

Data-parallel over 8 NeuronCores: the batch (8192) is sharded into 8
local shards of 1024 rows; all weights are replicated. The curvature
scalar c_avg couples the shards through a global mean, handled with a
single-scalar AllReduce.

Math (reference):
    h = tanh(x @ W1 + b1)
    u = sigmoid(h @ W2 + b2)
    c = clip(mean(MIN_C + (MAX_C-MIN_C) * sigmoid(relu(x@cp_w1.T+cp_b1)@cp_w2.T+cp_b2)), MIN_C, MAX_C)
    z = poincare_ball_layer(h, u, c, T)
    out = z @ Wo + bo

The poincare layer collapses algebraically to z = alpha(row)*h + beta(row)*u
where alpha/beta are scalar functions of the row statistics
x2=||h||^2, y2=||u||^2, xy=<h,u> and c.  The NaN fallback is measure-zero
and omitted.

Perf notes (measured on hw):
  - With 8 cores active the PE is power-throttled to ~2.0 GHz (HAM state
    31), so bf16 N=512 matmuls pace at ~263 ns.  The only lever is fewer
    PE cycles: MM2 (h@W2, the largest matmul) runs as fp8-e4m3 DoubleRow
    (256-deep contraction per pass, 2x FLOP rate, measured 216 ns/MM at
    2.4 GHz in isolation).  W2 is host-converted to e4m3 * 1024 (clipped
    to +-240, the TRN e4m3 max); the 1/1024 folds into the sigmoid's
    activation scale.  h is cast bf16->fp8 per row-tile on the DVE.
    MM1 / MMo stay bf16: fp8 there pushes rel-err too close to the 2e-2
    gate (simulated 1.96e-2 vs 1.26e-2 for MM2-only).
  - Row statistics go batch-major directly via tiny stationary-squares
    matmuls (lhsT = accumulated squares block, rhs = ones) instead of a
    feature-major ones-reduction + DRAM bounce, shortening the serial
    stats -> alpha/beta tail.  The per-row scalar chain runs once on
    [128, 8] tiles reading stats straight from SBUF.
"""

import tempfile
from contextlib import ExitStack

import numpy as np
import ml_dtypes

import concourse.bass as bass
import concourse.bacc as bacc
import concourse.mybir as mybir
import concourse.tile as tile
from concourse.bass_utils import run_bass_kernel_spmd

P = 128
N_CORES = 8
B_FULL = 8192
BL = B_FULL // N_CORES          # 1024 rows per core
IN = 3072
HID = 4096
OUT = 1000
KI = IN // P                    # 24
KH = HID // P                   # 32
NB = BL // P                    # 8 batch tiles
MIN_C = 0.001 * 0.5
MAX_C = 0.001 * 2.0
T_CONST = 0.7
EPS = 1e-7
W2S = 1024.0                    # host-side fp8 scale for W2
E4MAX = 240.0                   # TRN e4m3 saturation

dt = mybir.dt
AF = mybir.ActivationFunctionType
ALU = mybir.AluOpType
PM = mybir.MatmulPerfMode
BF = ml_dtypes.bfloat16
E4 = ml_dtypes.float8_e4m3

_nc_cache = []
_IDENT = np.eye(P, dtype=ml_dtypes.bfloat16)


def _build(with_b1, with_b2):
    nc = bacc.Bacc("TRN2", target_bir_lowering=False, debug=False,
                   num_devices=N_CORES)

    xT_d = nc.dram_tensor("xT", [KI, P, BL], dt.bfloat16, kind="ExternalInput")
    # W1 rows split in two halves for finer DMA/SBUF: w1h[mh, g, p, kk, q] =
    # W1[(g*12+kk)*128+p, mh*128+q]
    w1_d = nc.dram_tensor("w1", [KH, 2, P, KI // 2, P], dt.bfloat16,
                          kind="ExternalInput")
    # W2 in fp8 (x1024): w2r[mh, p, kh, q] = W2[kh*128+p, mh*128+q]
    w2_d = nc.dram_tensor("w2", [KH, P, KH, P], dt.float8e4, kind="ExternalInput")
    wo_d = nc.dram_tensor("wo", [KH, P, OUT], dt.bfloat16, kind="ExternalInput")
    cpw1_d = nc.dram_tensor("cpw1", [KI, P, 16], dt.bfloat16, kind="ExternalInput")
    cpw2_d = nc.dram_tensor("cpw2", [16, 1], dt.bfloat16, kind="ExternalInput")
    cpb1_d = nc.dram_tensor("cpb1", [16, 1], dt.float32, kind="ExternalInput")
    cpb2_d = nc.dram_tensor("cpb2", [1, 1], dt.float32, kind="ExternalInput")
    ident_d = nc.dram_tensor("ident", [P, P], dt.bfloat16, kind="ExternalInput")
    b1_d = nc.dram_tensor("b1", [P, KH], dt.float32, kind="ExternalInput") if with_b1 else None
    b2_d = nc.dram_tensor("b2", [P, KH], dt.float32, kind="ExternalInput") if with_b2 else None
    out_d = nc.dram_tensor("out", [BL, OUT], dt.float32, kind="ExternalOutput")

    f32 = dt.float32
    bf16 = dt.bfloat16
    fp8 = dt.float8e4

    with tile.TileContext(nc) as tc, ExitStack() as ctx:
        const = ctx.enter_context(tc.tile_pool(name="const", bufs=1))
        big = ctx.enter_context(tc.tile_pool(name="big", bufs=1))
        htp = ctx.enter_context(tc.tile_pool(name="htp", bufs=1))
        hqp = ctx.enter_context(tc.tile_pool(name="hqp", bufs=1))
        wp = ctx.enter_context(tc.tile_pool(name="wp", bufs=2))
        wop = ctx.enter_context(tc.tile_pool(name="wop", bufs=3))
        scr = ctx.enter_context(tc.tile_pool(name="scr", bufs=2))
        zzp = ctx.enter_context(tc.tile_pool(name="zzp", bufs=3))
        sacc = ctx.enter_context(tc.tile_pool(name="sacc", bufs=1))
        abp = ctx.enter_context(tc.tile_pool(name="abp", bufs=1))
        scal = ctx.enter_context(tc.tile_pool(name="scal", bufs=1))
        outp = ctx.enter_context(tc.tile_pool(name="outp", bufs=2))
        cpp = ctx.enter_context(tc.tile_pool(name="cpp", bufs=1))
        dram = ctx.enter_context(tc.tile_pool(name="dram", bufs=1, space="DRAM"))

        V = nc.vector
        S = nc.scalar

        def sc(name, shape=(P, 8), dtype=f32):
            return scal.tile(list(shape), dtype, name=name, tag=name)

        # ---------- persistent activations (feature-major) ----------
        onesb = const.tile([P, 1], bf16, name="onesb")
        nc.vector.memset(onesb, 1.0)
        onesf = const.tile([P, 1], f32, name="onesf")
        nc.vector.memset(onesf, 1.0)
        xT_sb = big.tile([P, KI, BL], bf16, name="xT_sb", tag="big",
                         padded_shape=[P, KH, BL])
        # first weight half-row issues on the sync queue before anything else;
        # xT streams on the gpsimd + scalar queues in parallel (the load is
        # HBM-bandwidth-bound, ~35us for 6.3MB with all 8 cores pulling)
        w1row0 = wp.tile([P, KI // 2, P], bf16, name="w1row", tag="w1")
        nc.sync.dma_start(out=w1row0, in_=w1_d[0, 0])
        xq = [nc.gpsimd, nc.scalar]
        for idx, (a, b) in enumerate(((0, 2), (2, 4), (4, 7), (7, 10),
                                      (10, 13), (13, 16), (16, 20), (20, 24))):
            xq[idx % 2].dma_start(
                out=xT_sb[:, a:b, :],
                in_=xT_d[a:b].rearrange("k p b -> p k b"))
        ident_sb = const.tile([P, P], bf16, name="ident_sb")
        nc.sync.dma_start(out=ident_sb, in_=ident_d[:, :])
        hT_sb = htp.tile([P, KH, BL], bf16, name="hT_sb")
        hTq_sb = hqp.tile([P, KH, BL], fp8, name="hTq_sb")
        if with_b1:
            b1_sb = const.tile([P, KH], f32, name="b1_sb")
            nc.sync.dma_start(out=b1_sb, in_=b1_d[:, :])
        if with_b2:
            b2_sb = const.tile([P, KH], f32, name="b2_sb")
            nc.sync.dma_start(out=b2_sb, in_=b2_d[:, :])

        with ExitStack() as ph1:
            mm = ph1.enter_context(tc.tile_pool(name="mm", bufs=3, space="PSUM"))
            stp = ph1.enter_context(tc.tile_pool(name="stp", bufs=1, space="PSUM"))
            # one bank shared by the batch-major stats (cols 0:8 x2,
            # 8:16 y2, 16:24 xy) and the HAM warm-keeper matmul target
            # (cols 128:512); plus the a/b transpose target bank
            stat_ps = stp.tile([P, 512], f32, name="stat_ps")
            dum_ps = stat_ps[:, 128:512]
            abT_ps = stp.tile([16, P], bf16, name="abT_ps")

            # ---------- MM1: hT = tanh(W1.T @ xT) (bf16) ----------
            x2a = sacc.tile([P, BL], bf16, name="x2a")
            y2a = sacc.tile([P, BL], bf16, name="y2a")
            xya = sacc.tile([P, BL], bf16, name="xya")
            with nc.named_scope("mm1"):
                for mh in range(KH):
                    ps = mm.tile([P, BL], f32, name="ps", tag="mm")
                    if mh == 0:
                        w1a = w1row0
                    else:
                        w1a = wp.tile([P, KI // 2, P], bf16, name="w1row",
                                      tag="w1")
                        nc.sync.dma_start(out=w1a, in_=w1_d[mh, 0])
                    w1b = wp.tile([P, KI // 2, P], bf16, name="w1row", tag="w1")
                    nc.sync.dma_start(out=w1b, in_=w1_d[mh, 1])
                    for ki in range(KI):
                        wrow = w1a if ki < 12 else w1b
                        kk = ki % 12
                        nc.tensor.matmul(ps[:, 0:512], lhsT=wrow[:, kk, :],
                                         rhs=xT_sb[:, ki, 0:512],
                                         start=(ki == 0), stop=(ki == KI - 1))
                        nc.tensor.matmul(ps[:, 512:BL], lhsT=wrow[:, kk, :],
                                         rhs=xT_sb[:, ki, 512:BL],
                                         start=(ki == 0), stop=(ki == KI - 1))
                    if with_b1:
                        S.activation(hT_sb[:, mh, :], ps, AF.Tanh,
                                     bias=b1_sb[:, mh:mh + 1])
                    else:
                        S.activation(hT_sb[:, mh, :], ps, AF.Tanh)
                    # fp8 copy for MM2's DoubleRow rhs
                    V.tensor_copy(hTq_sb[:, mh, :], hT_sb[:, mh, :])
                    hh = scr.tile([P, BL], bf16, name="hh", tag="hh")
                    S.activation(hh, hT_sb[:, mh, :], AF.Square)
                    if mh == 0:
                        V.tensor_copy(x2a, hh)
                    else:
                        V.tensor_add(x2a, x2a, hh)
                # x2 -> batch-major: out[b,0] = sum_p x2a[p, blk*128+b]
                for blk in range(NB):
                    nc.tensor.matmul(stat_ps[:, blk:blk + 1],
                                     lhsT=x2a[:, blk * P:(blk + 1) * P],
                                     rhs=onesb, start=True, stop=True,
                                     skip_group_check=True)

            # ---------- curvature predictor (xT still resident; the
            # AllReduce hides under MM1/MM2) ----------
            with nc.named_scope("cp"):
                cpw1_sb = const.tile([P, KI, 16], bf16, name="cpw1_sb")
                nc.sync.dma_start(out=cpw1_sb,
                                  in_=cpw1_d.rearrange("k p q -> p k q"))
                cpw2_sb = const.tile([16, 1], bf16, name="cpw2_sb")
                nc.sync.dma_start(out=cpw2_sb, in_=cpw2_d[:, :])
                cpb1_sb = const.tile([16, 1], f32, name="cpb1_sb")
                nc.sync.dma_start(out=cpb1_sb, in_=cpb1_d[:, :])
                cpb2_sb = const.tile([1, 1], f32, name="cpb2_sb")
                nc.sync.dma_start(out=cpb2_sb, in_=cpb2_d[:, :])
                cph_sb = cpp.tile([16, BL], bf16, name="cph_sb")
                for ch in range(2):
                    cps = mm.tile([16, 512], f32, name="cps", tag="mm")
                    for ki in range(KI):
                        nc.tensor.matmul(
                            cps, lhsT=cpw1_sb[:, ki, :],
                            rhs=xT_sb[:, ki, ch * 512:(ch + 1) * 512],
                            start=(ki == 0), stop=(ki == KI - 1))
                    S.activation(cph_sb[:, ch * 512:(ch + 1) * 512], cps,
                                 AF.Relu, bias=cpb1_sb)
                sparts = []
                for ch in range(2):
                    c2p = mm.tile([1, 512], f32, name="c2p", tag="mm")
                    nc.tensor.matmul(c2p, lhsT=cpw2_sb,
                                     rhs=cph_sb[:16, ch * 512:(ch + 1) * 512],
                                     start=True, stop=True)
                    cpw = cpp.tile([1, 512], bf16, name="cpw", tag="cpw")
                    spart = cpp.tile([1, 1], f32, name=f"spart{ch}",
                                     tag=f"spart{ch}")
                    S.activation(cpw, c2p, AF.Sigmoid, bias=cpb2_sb,
                                 accum_out=spart)
                    sparts.append(spart)
                s_loc = cpp.tile([1, 1], f32, name="s_loc")
                V.tensor_add(s_loc, sparts[0], sparts[1])
                cin = dram.tile([1, 1], f32, name="cin")
                cout = dram.tile([1, 1], f32, name="cout")
                nc.sync.dma_start(out=cin, in_=s_loc)
                nc.gpsimd.collective_compute(
                    "AllReduce", ALU.add,
                    replica_groups=[list(range(N_CORES))],
                    ins=[cin.opt()], outs=[cout.opt()])
                s_b = sc("s_b", (P, 1))
                nc.gpsimd.dma_start(out=s_b, in_=cout.to_broadcast([P, 1]))
                # c = clip(MIN_C + (MAX_C-MIN_C)*mean(c_pred))
                c_b = sc("c_b", (P, 1))
                V.tensor_scalar(out=c_b, in0=s_b,
                                scalar1=(MAX_C - MIN_C) / B_FULL,
                                scalar2=MIN_C, op0=ALU.mult, op1=ALU.add)
                V.tensor_scalar_min(out=c_b, in0=c_b, scalar1=MAX_C)
                V.tensor_scalar_max(out=c_b, in0=c_b, scalar1=MIN_C)
                negc_b = sc("negc_b", (P, 1))
                V.tensor_scalar_mul(out=negc_b, in0=c_b, scalar1=-1.0)
                twoc_b = sc("twoc_b", (P, 1))
                V.tensor_scalar_mul(out=twoc_b, in0=c_b, scalar1=2.0)
                neg2c_b = sc("neg2c_b", (P, 1))
                V.tensor_scalar_mul(out=neg2c_b, in0=c_b, scalar1=-2.0)
                c2_b = sc("c2_b", (P, 1))
                V.tensor_mul(c2_b, c_b, c_b)

            # ---------- MM2: uT = sigmoid(W2.T @ hT) fp8 DoubleRow ----------
            uT_sb = big.tile([P, KH, BL], bf16, name="uT_sb", tag="big")
            inv_s2 = 1.0 / W2S
            with nc.named_scope("mm2"):
                for mh in range(KH):
                    ps = mm.tile([P, BL], f32, name="ps", tag="mm")
                    w2row = wp.tile([P, KH, P], fp8, name="w2row", tag="w2")
                    nc.sync.dma_start(out=w2row, in_=w2_d[mh])
                    for j in range(KH // 2):
                        nc.tensor.matmul(ps[:, 0:512],
                                         lhsT=w2row[:, 2 * j:2 * j + 2, :],
                                         rhs=hTq_sb[:, 2 * j:2 * j + 2, 0:512],
                                         start=(j == 0), stop=(j == KH // 2 - 1),
                                         perf_mode=PM.DoubleRow)
                        nc.tensor.matmul(ps[:, 512:BL],
                                         lhsT=w2row[:, 2 * j:2 * j + 2, :],
                                         rhs=hTq_sb[:, 2 * j:2 * j + 2, 512:BL],
                                         start=(j == 0), stop=(j == KH // 2 - 1),
                                         perf_mode=PM.DoubleRow)
                    if with_b2:
                        S.activation(uT_sb[:, mh, :], ps, AF.Sigmoid,
                                     bias=b2_sb[:, mh:mh + 1], scale=inv_s2)
                    else:
                        S.activation(uT_sb[:, mh, :], ps, AF.Sigmoid,
                                     scale=inv_s2)
                    uu = scr.tile([P, BL], bf16, name="uu", tag="hh")
                    S.activation(uu, uT_sb[:, mh, :], AF.Square)
                    hu = scr.tile([P, BL], bf16, name="hu", tag="hh")
                    V.tensor_mul(hu, hT_sb[:, mh, :], uT_sb[:, mh, :])
                    if mh == 0:
                        V.tensor_copy(y2a, uu)
                        V.tensor_copy(xya, hu)
                    else:
                        V.tensor_add(y2a, y2a, uu)
                        V.tensor_add(xya, xya, hu)
                # warm-keeper group 1: bridges the PE from mm2's last matmul
                # across the stats-accumulation drain (a >3.4us idle would
                # HAM-throttle the clock for the MMo matmuls).  A real
                # accumulation group so DCE cannot drop it.
                for i in range(14):
                    nc.tensor.matmul(dum_ps, lhsT=ident_sb,
                                     rhs=hT_sb[:, i, 0:384],
                                     start=(i == 0), stop=(i == 13),
                                     skip_group_check=True)
                for blk in range(NB):
                    nc.tensor.matmul(stat_ps[:, 8 + blk:9 + blk],
                                     lhsT=y2a[:, blk * P:(blk + 1) * P],
                                     rhs=onesb, start=True, stop=True,
                                     skip_group_check=True)
                    nc.tensor.matmul(stat_ps[:, 16 + blk:17 + blk],
                                     lhsT=xya[:, blk * P:(blk + 1) * P],
                                     rhs=onesb, start=True, stop=True,
                                     skip_group_check=True)

            # ---------- per-row scalar chain, batch-major [128, 8] ----------
            alpha_b = abp.tile([P, BL], bf16, name="alpha_b")
            beta_b = abp.tile([P, BL], bf16, name="beta_b")
            ab_d = dram.tile([2, BL], bf16, name="ab_d")

            with nc.named_scope("scalars"):
                stats_sb = sc("stats_sb", (P, 24))
                V.tensor_copy(stats_sb, stat_ps[:, 0:24])
                x2 = stats_sb[:, 0:8]
                y2 = stats_sb[:, 8:16]
                xy = stats_sb[:, 16:24]
                w = sc("w")
                V.scalar_tensor_tensor(out=w, in0=xy, scalar=-2.0, in1=y2,
                                       op0=ALU.mult, op1=ALU.add)
                A1 = sc("A1")
                V.tensor_scalar(out=A1, in0=w, scalar1=c_b, scalar2=1.0,
                                op0=ALU.mult, op1=ALU.add)
                A2 = sc("A2")
                V.tensor_scalar(out=A2, in0=x2, scalar1=negc_b, scalar2=1.0,
                                op0=ALU.mult, op1=ALU.add)
                p1 = sc("p1")
                V.tensor_mul(p1, x2, y2)
                den = sc("den")
                V.tensor_scalar(out=den, in0=p1, scalar1=c2_b, scalar2=1.0,
                                op0=ALU.mult, op1=ALU.add)
                V.scalar_tensor_tensor(out=den, in0=xy, scalar=neg2c_b, in1=den,
                                       op0=ALU.mult, op1=ALU.add)
                V.tensor_scalar_add(out=den, in0=den, scalar1=EPS)
                D = sc("D")
                V.reciprocal(D, den)
                nc.tensor.matmul(stat_ps[0:1, 80:88], lhsT=onesf, rhs=D,
                                 start=True, stop=True, skip_group_check=True)
                # ||a||^2 = D^2 (A1^2 x2 - 2 A1 A2 xy + A2^2 y2)
                t1 = sc("t1")
                V.tensor_mul(t1, A1, A1)
                V.tensor_mul(t1, t1, x2)
                t2 = sc("t2")
                V.tensor_mul(t2, A1, A2)
                V.tensor_mul(t2, t2, xy)
                t3 = sc("t3")
                V.tensor_mul(t3, A2, A2)
                V.tensor_mul(t3, t3, y2)
                na2 = sc("na2")
                V.scalar_tensor_tensor(out=na2, in0=t2, scalar=-2.0, in1=t1,
                                       op0=ALU.mult, op1=ALU.add)
                V.tensor_add(na2, na2, t3)
                dsq = sc("dsq")
                V.tensor_mul(dsq, D, D)
                V.tensor_mul(na2, na2, dsq)
                # q = sqrt(c * na2) with one Newton step (ACT sqrt is low precision)
                q2 = sc("q2")
                V.tensor_scalar(out=q2, in0=na2, scalar1=c_b, scalar2=None,
                                op0=ALU.mult)
                q0 = sc("q0")
                S.activation(q0, q2, AF.Sqrt)
                V.tensor_scalar_max(out=q0, in0=q0, scalar1=1e-20)
                r0 = sc("r0")
                V.reciprocal(r0, q0)
                q = sc("q")
                V.tensor_mul(q, q2, r0)
                V.tensor_add(q, q, q0)
                V.tensor_scalar_mul(out=q, in0=q, scalar1=0.5)
                arg = sc("arg")
                V.tensor_scalar_min(out=arg, in0=q, scalar1=1.0 - 1e-5)
                # artanh(arg) = 0.5 ln((1+arg)/(1-arg)); t = tanh(T*artanh)/q
                opp = sc("opp")
                V.tensor_scalar(out=opp, in0=arg, scalar1=-1.0, scalar2=1.0,
                                op0=ALU.mult, op1=ALU.add)
                opn = sc("opn")
                V.tensor_scalar_add(out=opn, in0=arg, scalar1=1.0)
                rr = sc("rr")
                V.reciprocal(rr, opp)
                rat = sc("rat")
                V.tensor_mul(rat, opn, rr)
                lg = sc("lg")
                S.activation(lg, rat, AF.Ln)
                th = sc("th")
                S.activation(th, lg, AF.Tanh, scale=T_CONST * 0.5)
                rq = sc("rq")
                V.reciprocal(rq, q)
                tm = sc("tm")
                V.tensor_mul(tm, th, rq)
                nc.tensor.matmul(stat_ps[0:1, 88:96], lhsT=onesf, rhs=tm,
                                 start=True, stop=True, skip_group_check=True)
                # <h,a> = D (A2 xy - A1 x2)
                s1_ = sc("s1_")
                V.tensor_mul(s1_, A1, x2)
                s2_ = sc("s2_")
                V.tensor_mul(s2_, A2, xy)
                ha = sc("ha")
                V.tensor_sub(ha, s2_, s1_)
                V.tensor_mul(ha, ha, D)
                hm = sc("hm")
                V.tensor_mul(hm, tm, ha)
                tsq = sc("tsq")
                V.tensor_mul(tsq, tm, tm)
                m2 = sc("m2")
                V.tensor_mul(m2, tsq, na2)
                w2s = sc("w2s")
                V.scalar_tensor_tensor(out=w2s, in0=hm, scalar=2.0, in1=m2,
                                       op0=ALU.mult, op1=ALU.add)
                B1 = sc("B1")
                V.tensor_scalar(out=B1, in0=w2s, scalar1=c_b, scalar2=1.0,
                                op0=ALU.mult, op1=ALU.add)
                p2 = sc("p2")
                V.tensor_mul(p2, x2, m2)
                den2 = sc("den2")
                V.tensor_scalar(out=den2, in0=p2, scalar1=c2_b, scalar2=1.0,
                                op0=ALU.mult, op1=ALU.add)
                V.scalar_tensor_tensor(out=den2, in0=hm, scalar=twoc_b, in1=den2,
                                       op0=ALU.mult, op1=ALU.add)
                V.tensor_scalar_add(out=den2, in0=den2, scalar1=EPS)
                D2 = sc("D2")
                V.reciprocal(D2, den2)
                g = sc("g")
                V.tensor_mul(g, A2, tm)
                V.tensor_mul(g, g, D)
                w3 = sc("w3")
                V.tensor_mul(w3, g, A1)
                V.tensor_sub(w3, B1, w3)
                # alpha -> cols 0:8, beta -> cols 8:16 of one bf16 tile; a PE
                # transpose then yields batch-linear rows for a fast DMA
                # bounce (8 contiguous 256B descriptors instead of a 2-byte
                # element scatter).
                ab_bm = sc("ab_bm", (P, 16), bf16)
                V.tensor_mul(ab_bm[:, 0:8], w3, D2)
                w4 = sc("w4")
                V.tensor_mul(w4, g, A2)
                V.tensor_mul(ab_bm[:, 8:16], w4, D2)
                nc.tensor.transpose(abT_ps, ab_bm, ident_sb)
                # warm-keeper taps pinned after the chain end: the scheduler
                # hoists dependency-free matmuls, so these read ab_bm
                for i in range(3):
                    nc.tensor.matmul(stat_ps[:, 32 + 16 * i:48 + 16 * i],
                                     lhsT=ident_sb, rhs=ab_bm,
                                     start=True, stop=True,
                                     skip_group_check=True)
                abT_sb = sc("abT_sb", (16, P), bf16)
                V.tensor_copy(abT_sb, abT_ps)
                nc.sync.dma_start(out=ab_d[0, :].rearrange("(j b) -> j b", j=8),
                                  in_=abT_sb[0:8, :])
                nc.sync.dma_start(out=ab_d[1, :].rearrange("(j b) -> j b", j=8),
                                  in_=abT_sb[8:16, :])
                for hsl in (slice(0, 512), slice(512, BL)):
                    nc.scalar.dma_start(out=alpha_b[:, hsl],
                                        in_=ab_d[0:1, hsl].to_broadcast([P, 512]))
                    nc.sync.dma_start(out=beta_b[:, hsl],
                                      in_=ab_d[1:2, hsl].to_broadcast([P, 512]))
                for i in range(2):
                    nc.tensor.matmul(dum_ps, lhsT=ident_sb,
                                     rhs=(alpha_b[:, 0:384] if i == 0
                                          else beta_b[:, 512:896]),
                                     start=(i == 0), stop=(i == 1),
                                     skip_group_check=True)
                # anchor all warm-keeper matmuls against DCE
                dum_sb = sc("dum_sb", (1, 128), bf16)
                V.tensor_copy(dum_sb, stat_ps[0:1, 24:152])
                dum_d = dram.tile([1, 128], bf16, name="dum_d")
                nc.sync.dma_start(out=dum_d, in_=dum_sb)

        # psum pools (mm, stp) released here

        # ---------- z = alpha*h + beta*u (overwrites uT in place),
        # then out = z @ Wo.  Processed in two batch-column halves so the
        # MMo matmuls of half 0 overlap the DVE z-combine of half 1.
        with ExitStack() as ph2:
            mmo = ph2.enter_context(tc.tile_pool(name="mmo", bufs=8,
                                                 space="PSUM"))
            for bg in range(2):
                csl = slice(bg * 512, (bg + 1) * 512)
                with nc.named_scope(f"zcomb{bg}"):
                    for kh in range(KH):
                        t1z = zzp.tile([P, 512], bf16, name="t1z", tag="zz")
                        V.tensor_mul(t1z, hT_sb[:, kh, csl], alpha_b[:, csl])
                        t2z = zzp.tile([P, 512], bf16, name="t2z", tag="zz")
                        V.tensor_mul(t2z, uT_sb[:, kh, csl], beta_b[:, csl])
                        V.tensor_add(uT_sb[:, kh, csl], t1z, t2z)
                with nc.named_scope(f"mmo{bg}"):
                    pso = [mmo.tile([P, 500], f32, name=f"pso{bg}_{i}",
                                    tag="mmo") for i in range(8)]
                    for kh in range(KH):
                        wot = wop.tile([P, OUT], bf16, name="wot", tag="wo")
                        nc.sync.dma_start(out=wot, in_=wo_d[kh])
                        for i in range(4):
                            b = bg * 4 + i
                            for och in range(2):
                                nc.tensor.matmul(
                                    pso[i * 2 + och],
                                    lhsT=uT_sb[:, kh, b * P:(b + 1) * P],
                                    rhs=wot[:, och * 500:(och + 1) * 500],
                                    start=(kh == 0), stop=(kh == KH - 1))
                    for i in range(4):
                        b = bg * 4 + i
                        for och in range(2):
                            ob = outp.tile([P, 500], f32, name="ob", tag="ob")
                            if och == 0:
                                S.copy(ob, pso[i * 2])
                            else:
                                V.tensor_copy(ob, pso[i * 2 + 1])
                            outq = nc.sync if och == 0 else nc.scalar
                            outq.dma_start(
                                out=out_d[b * P:(b + 1) * P,
                                          och * 500:(och + 1) * 500],
                                in_=ob)

    nc.compile()
    return nc


def _get_nc(with_b1, with_b2):
    for k, v in _nc_cache:
        if k == (with_b1, with_b2):
            return v
    nc = _build(with_b1, with_b2)
    _nc_cache.append(((with_b1, with_b2), nc))
    return nc


def kernel(x, W1, b1, W2, b2, Wo, bo, cp_w1, cp_b1, cp_w2, cp_b2,
           _trace=False, _tmpdir=None):
    x = np.asarray(x, dtype=np.float32)
    with_b1 = bool(np.any(b1))
    with_b2 = bool(np.any(b2))
    nc = _get_nc(with_b1, with_b2)

    # w1h[mh, g, p, kk, q] = W1[(g*12+kk)*128+p, mh*128+q]
    w1_t = np.ascontiguousarray(
        np.asarray(W1, np.float32).reshape(KI, P, KH, P).transpose(2, 1, 0, 3)
        .reshape(KH, P, 2, KI // 2, P).transpose(0, 2, 1, 3, 4)
    ).astype(BF)
    # w2r[mh, p, kh, q] = W2[kh*128+p, mh*128+q], fp8 e4m3 scaled x1024
    w2_t = np.ascontiguousarray(
        np.clip(np.asarray(W2, np.float32) * W2S, -E4MAX, E4MAX)
        .reshape(KH, P, KH, P).transpose(2, 1, 0, 3)
    ).astype(E4)
    wo_t = np.asarray(Wo, np.float32).reshape(KH, P, OUT).astype(BF)
    cpw1_t = np.ascontiguousarray(
        np.asarray(cp_w1, np.float32).T.reshape(KI, P, 16)).astype(BF)
    cpw2_t = np.asarray(cp_w2, np.float32).reshape(1, 16).T.astype(BF)
    cpw2_t = np.ascontiguousarray(cpw2_t)
    cpb1_t = np.asarray(cp_b1, np.float32).reshape(16, 1)
    cpb2_t = np.asarray(cp_b2, np.float32).reshape(1, 1)
    b1_t = np.ascontiguousarray(np.asarray(b1, np.float32).reshape(KH, P).T)
    b2_t = np.ascontiguousarray(np.asarray(b2, np.float32).reshape(KH, P).T)

    in_maps = []
    for c in range(N_CORES):
        shard = x[c * BL:(c + 1) * BL]
        xT = np.ascontiguousarray(shard.T).reshape(KI, P, BL).astype(BF)
        m = {"xT": xT, "w1": w1_t, "w2": w2_t, "wo": wo_t,
             "cpw1": cpw1_t, "cpw2": cpw2_t, "cpb1": cpb1_t, "cpb2": cpb2_t,
             "ident": _IDENT}
        if with_b1:
            m["b1"] = b1_t
        if with_b2:
            m["b2"] = b2_t
        in_maps.append(m)

    kw = {}
    if _trace:
        kw = dict(trace=True, tmpdir=_tmpdir or tempfile.mkdtemp(prefix="cdk_"))
    res = run_bass_kernel_spmd(nc, in_maps, list(range(N_CORES)), **kw)

    out = np.concatenate([res.results[c]["out"] for c in range(N_CORES)], axis=0)
    bo = np.asarray(bo, np.float32)
    if np.any(bo):
        out = out + bo
    if _trace:
        kernel._last_result = res
    return out


# revision 19
# speedup vs baseline: 1.3610x; 1.0283x over previous
"""Trainium2 Bass kernel for nn_ConservativeDynamicCurvatureMLP.

Data-parallel over 8 NeuronCores: the batch (8192) is sharded into 8
local shards of 1024 rows; all weights are replicated. The curvature
scalar c_avg couples the shards through a global mean, handled with a
single-scalar AllReduce.

Math (reference):
    h = tanh(x @ W1 + b1)
    u = sigmoid(h @ W2 + b2)
    c = clip(mean(MIN_C + (MAX_C-MIN_C) * sigmoid(relu(x@cp_w1.T+cp_b1)@cp_w2.T+cp_b2)), MIN_C, MAX_C)
    z = poincare_ball_layer(h, u, c, T)
    out = z @ Wo + bo

The poincare layer collapses algebraically to z = alpha(row)*h + beta(row)*u
where alpha/beta are scalar functions of the row statistics
x2=||h||^2, y2=||u||^2, xy=<h,u> and c.  The NaN fallback is measure-zero
and omitted.

Perf notes (measured on hw):
  - With 8 cores active the PE is power-throttled to ~2.0 GHz (HAM state
    31), so bf16 N=512 matmuls pace at ~263 ns.  The only lever is fewer
    PE cycles: MM2 (h@W2, the largest matmul) runs as fp8-e4m3 DoubleRow
    (256-deep contraction per pass, 2x FLOP rate, measured 216 ns/MM at
    2.4 GHz in isolation).  W2 is host-converted to e4m3 * 1024 (clipped
    to +-240, the TRN e4m3 max); the 1/1024 folds into the sigmoid's
    activation scale.  h is cast bf16->fp8 per row-tile on the DVE.
    MM1 / MMo stay bf16: fp8 there pushes rel-err too close to the 2e-2
    gate (simulated 1.96e-2 vs 1.26e-2 for MM2-only).
  - Row statistics go batch-major directly via tiny stationary-squares
    matmuls (lhsT = accumulated squares block, rhs = ones) instead of a
    feature-major ones-reduction + DRAM bounce, shortening the serial
    stats -> alpha/beta tail.  The per-row scalar chain runs once on
    [128, 8] tiles reading stats straight from SBUF.
"""

import tempfile
from contextlib import ExitStack

import numpy as np
import ml_dtypes

import concourse.bass as bass
import concourse.bacc as bacc
import concourse.mybir as mybir
import concourse.tile as tile
from concourse.bass_utils import run_bass_kernel_spmd

P = 128
N_CORES = 8
B_FULL = 8192
BL = B_FULL // N_CORES          # 1024 rows per core
IN = 3072
HID = 4096
OUT = 1000
KI = IN // P                    # 24
KQ = 8                          # leading ki-groups of MM1 done in fp8
KB = KI - KQ                    # trailing ki-groups in bf16
KH = HID // P                   # 32
NB = BL // P                    # 8 batch tiles
MIN_C = 0.001 * 0.5
MAX_C = 0.001 * 2.0
T_CONST = 0.7
EPS = 1e-7
W2S = 1024.0                    # host-side fp8 scale for W2
E4MAX = 240.0                   # TRN e4m3 saturation

dt = mybir.dt
AF = mybir.ActivationFunctionType
ALU = mybir.AluOpType
PM = mybir.MatmulPerfMode
BF = ml_dtypes.bfloat16
E4 = ml_dtypes.float8_e4m3

_nc_cache = []
_IDENT = np.eye(P, dtype=ml_dtypes.bfloat16)


def _build(with_b1, with_b2):
    nc = bacc.Bacc("TRN2", target_bir_lowering=False, debug=False,
                   num_devices=N_CORES)

    # x features 0:1024 as fp8 (partial-fp8 MM1), 1024:3072 as bf16
    xq_d = nc.dram_tensor("xq", [KQ, P, BL], dt.float8e4, kind="ExternalInput")
    xT_d = nc.dram_tensor("xT", [KB, P, BL], dt.bfloat16, kind="ExternalInput")
    # W1 scaled x1024 throughout (so fp8 and bf16 parts share one psum
    # accumulator); fp8 rows in DoubleRow pair layout, bf16 rows in halves
    w1q_d = nc.dram_tensor("w1q", [KH, P, KQ // 2, 2, P], dt.float8e4,
                           kind="ExternalInput")
    w1_d = nc.dram_tensor("w1", [KH, 2, P, KB // 2, P], dt.bfloat16,
                          kind="ExternalInput")
    # W2 in fp8 (x1024): w2r[mh, p, kh, q] = W2[kh*128+p, mh*128+q]
    w2_d = nc.dram_tensor("w2", [KH, P, KH, P], dt.float8e4, kind="ExternalInput")
    wo_d = nc.dram_tensor("wo", [KH, P, OUT], dt.bfloat16, kind="ExternalInput")
    cpw1_d = nc.dram_tensor("cpw1", [KI, P, 16], dt.float8e4, kind="ExternalInput")
    cpw2_d = nc.dram_tensor("cpw2", [16, 1], dt.bfloat16, kind="ExternalInput")
    cpb1_d = nc.dram_tensor("cpb1", [16, 1], dt.float32, kind="ExternalInput")
    cpb2_d = nc.dram_tensor("cpb2", [1, 1], dt.float32, kind="ExternalInput")
    ident_d = nc.dram_tensor("ident", [P, P], dt.bfloat16, kind="ExternalInput")
    b1_d = nc.dram_tensor("b1", [P, KH], dt.float32, kind="ExternalInput") if with_b1 else None
    b2_d = nc.dram_tensor("b2", [P, KH], dt.float32, kind="ExternalInput") if with_b2 else None
    out_d = nc.dram_tensor("out", [BL, OUT], dt.float32, kind="ExternalOutput")

    f32 = dt.float32
    bf16 = dt.bfloat16
    fp8 = dt.float8e4

    with tile.TileContext(nc) as tc, ExitStack() as ctx:
        const = ctx.enter_context(tc.tile_pool(name="const", bufs=1))
        big = ctx.enter_context(tc.tile_pool(name="big", bufs=1))
        htp = ctx.enter_context(tc.tile_pool(name="htp", bufs=1))
        hqp = ctx.enter_context(tc.tile_pool(name="hqp", bufs=1))
        wp = ctx.enter_context(tc.tile_pool(name="wp", bufs=2))
        wop = ctx.enter_context(tc.tile_pool(name="wop", bufs=2))
        scr = ctx.enter_context(tc.tile_pool(name="scr", bufs=2))
        xqp = ctx.enter_context(tc.tile_pool(name="xqp", bufs=1))
        zzp = ctx.enter_context(tc.tile_pool(name="zzp", bufs=3))
        sacc = ctx.enter_context(tc.tile_pool(name="sacc", bufs=1))
        abp = ctx.enter_context(tc.tile_pool(name="abp", bufs=1))
        scal = ctx.enter_context(tc.tile_pool(name="scal", bufs=1))
        cpp = ctx.enter_context(tc.tile_pool(name="cpp", bufs=1))
        dram = ctx.enter_context(tc.tile_pool(name="dram", bufs=1, space="DRAM"))

        V = nc.vector
        S = nc.scalar

        def sc(name, shape=(P, 8), dtype=f32):
            return scal.tile(list(shape), dtype, name=name, tag=name)

        # ---------- persistent activations (feature-major) ----------
        onesb = const.tile([P, 1], bf16, name="onesb")
        nc.vector.memset(onesb, 1.0)
        onesf = const.tile([P, 1], f32, name="onesf")
        nc.vector.memset(onesf, 1.0)
        xT_sb = big.tile([P, KB, BL], bf16, name="xT_sb", tag="big",
                         padded_shape=[P, KH, BL])
        # first weight half-row issues on the sync queue before anything else;
        # xT streams on the gpsimd + scalar queues in parallel (the load is
        # HBM-bandwidth-bound, ~35us for 6.3MB with all 8 cores pulling)
        w1q0 = wp.tile([P, KQ // 2, 2, P], fp8, name="w1qrow", tag="w1q")
        nc.sync.dma_start(out=w1q0, in_=w1q_d[0])
        xq_sb = xqp.tile([P, KQ, BL], fp8, name="xq_sb")
        queues = [nc.scalar, nc.gpsimd]
        for idx, (a, b) in enumerate(((0, 2), (2, 4), (4, 6), (6, 8))):
            queues[idx % 2].dma_start(
                out=xq_sb[:, a:b, :],
                in_=xq_d[a:b].rearrange("k p b -> p k b"))
        for idx, (a, b) in enumerate(((0, 2), (2, 4), (4, 7), (7, 10),
                                      (10, 13), (13, 16))):
            queues[idx % 2].dma_start(
                out=xT_sb[:, a:b, :],
                in_=xT_d[a:b].rearrange("k p b -> p k b"))
        ident_sb = const.tile([P, P], bf16, name="ident_sb")
        nc.sync.dma_start(out=ident_sb, in_=ident_d[:, :])
        hT_sb = htp.tile([P, KH, BL], bf16, name="hT_sb")
        hTq_sb = hqp.tile([P, KH, BL], fp8, name="hTq_sb")
        if with_b1:
            b1_sb = const.tile([P, KH], f32, name="b1_sb")
            nc.sync.dma_start(out=b1_sb, in_=b1_d[:, :])
        if with_b2:
            b2_sb = const.tile([P, KH], f32, name="b2_sb")
            nc.sync.dma_start(out=b2_sb, in_=b2_d[:, :])

        with ExitStack() as ph1:
            mm = ph1.enter_context(tc.tile_pool(name="mm", bufs=3, space="PSUM"))
            stp = ph1.enter_context(tc.tile_pool(name="stp", bufs=1, space="PSUM"))
            # one bank shared by the batch-major stats (cols 0:8 x2,
            # 8:16 y2, 16:24 xy) and the HAM warm-keeper matmul target
            # (cols 128:512); plus the a/b transpose target bank
            stat_ps = stp.tile([P, 512], f32, name="stat_ps")
            dum_ps = stat_ps[:, 128:512]
            abT_ps = stp.tile([16, P], bf16, name="abT_ps")

            # ---------- MM1: hT = tanh(W1.T @ xT) (bf16) ----------
            x2a = sacc.tile([P, BL], bf16, name="x2a")
            y2a = sacc.tile([P, BL], bf16, name="y2a")
            xya = sacc.tile([P, BL], bf16, name="xya")
            with nc.named_scope("mm1"):
                for mh in range(KH):
                    ps = mm.tile([P, BL], f32, name="ps", tag="mm")
                    if mh == 0:
                        w1q = w1q0
                    else:
                        w1q = wp.tile([P, KQ // 2, 2, P], fp8, name="w1qrow",
                                      tag="w1q")
                        nc.sync.dma_start(out=w1q, in_=w1q_d[mh])
                    w1a = wp.tile([P, KB // 2, P], bf16, name="w1row", tag="w1")
                    nc.sync.dma_start(out=w1a, in_=w1_d[mh, 0])
                    w1b = wp.tile([P, KB // 2, P], bf16, name="w1row", tag="w1")
                    nc.sync.dma_start(out=w1b, in_=w1_d[mh, 1])
                    for hsl in (slice(0, 512), slice(512, BL)):
                        for j in range(KQ // 2):
                            nc.tensor.matmul(
                                ps[:, hsl], lhsT=w1q[:, j, :, :],
                                rhs=xq_sb[:, 2 * j:2 * j + 2, hsl],
                                start=(j == 0), stop=False,
                                perf_mode=PM.DoubleRow,
                                skip_group_check=True)
                        for ki in range(KB):
                            wrow = w1a if ki < KB // 2 else w1b
                            kk = ki % (KB // 2)
                            nc.tensor.matmul(ps[:, hsl], lhsT=wrow[:, kk, :],
                                             rhs=xT_sb[:, ki, hsl],
                                             start=False, stop=(ki == KB - 1),
                                             skip_group_check=True)
                    if with_b1:
                        S.activation(hT_sb[:, mh, :], ps, AF.Tanh,
                                     bias=b1_sb[:, mh:mh + 1], scale=1.0 / W2S)
                    else:
                        S.activation(hT_sb[:, mh, :], ps, AF.Tanh,
                                     scale=1.0 / W2S)
                    # fp8 copy for MM2's DoubleRow rhs
                    V.tensor_copy(hTq_sb[:, mh, :], hT_sb[:, mh, :])
                    hh = scr.tile([P, BL], bf16, name="hh", tag="hh")
                    S.activation(hh, hT_sb[:, mh, :], AF.Square)
                    if mh == 0:
                        V.tensor_copy(x2a, hh)
                    else:
                        V.tensor_add(x2a, x2a, hh)
                # x2 -> batch-major: out[b,0] = sum_p x2a[p, blk*128+b]
                for blk in range(NB):
                    nc.tensor.matmul(stat_ps[:, blk:blk + 1],
                                     lhsT=x2a[:, blk * P:(blk + 1) * P],
                                     rhs=onesb, start=True, stop=True,
                                     skip_group_check=True)

            # ---------- curvature predictor (xT still resident; the
            # AllReduce hides under MM1/MM2) ----------
            with nc.named_scope("cp"):
                cpw1_sb = const.tile([P, KI, 16], fp8, name="cpw1_sb")
                nc.sync.dma_start(out=cpw1_sb,
                                  in_=cpw1_d.rearrange("k p q -> p k q"))
                cpw2_sb = const.tile([16, 1], bf16, name="cpw2_sb")
                nc.sync.dma_start(out=cpw2_sb, in_=cpw2_d[:, :])
                cpb1_sb = const.tile([16, 1], f32, name="cpb1_sb")
                nc.sync.dma_start(out=cpb1_sb, in_=cpb1_d[:, :])
                cpb2_sb = const.tile([1, 1], f32, name="cpb2_sb")
                nc.sync.dma_start(out=cpb2_sb, in_=cpb2_d[:, :])
                cph_sb = cpp.tile([16, BL], fp8, name="cph_sb")
                for ch in range(2):
                    cps = mm.tile([16, 512], f32, name="cps", tag="mm")
                    for ki in range(KI):
                        xr = (xq_sb[:, ki, ch * 512:(ch + 1) * 512] if ki < KQ
                              else xT_sb[:, ki - KQ, ch * 512:(ch + 1) * 512])
                        nc.tensor.matmul(
                            cps, lhsT=cpw1_sb[:, ki, :], rhs=xr,
                            start=(ki == 0), stop=(ki == KI - 1))
                    S.activation(cph_sb[:, ch * 512:(ch + 1) * 512], cps,
                                 AF.Relu, bias=cpb1_sb, scale=1.0 / 64.0)
                sparts = []
                for ch in range(2):
                    c2p = mm.tile([1, 512], f32, name="c2p", tag="mm")
                    nc.tensor.matmul(c2p, lhsT=cpw2_sb,
                                     rhs=cph_sb[:16, ch * 512:(ch + 1) * 512],
                                     start=True, stop=True)
                    cpw = cpp.tile([1, 512], bf16, name="cpw", tag="cpw")
                    spart = cpp.tile([1, 1], f32, name=f"spart{ch}",
                                     tag=f"spart{ch}")
                    S.activation(cpw, c2p, AF.Sigmoid, bias=cpb2_sb,
                                 accum_out=spart)
                    sparts.append(spart)
                s_loc = cpp.tile([1, 1], f32, name="s_loc")
                V.tensor_add(s_loc, sparts[0], sparts[1])
                cin = dram.tile([1, 1], f32, name="cin")
                cout = dram.tile([1, 1], f32, name="cout")
                nc.sync.dma_start(out=cin, in_=s_loc)
                nc.gpsimd.collective_compute(
                    "AllReduce", ALU.add,
                    replica_groups=[list(range(N_CORES))],
                    ins=[cin.opt()], outs=[cout.opt()])
                s_b = sc("s_b", (P, 1))
                nc.gpsimd.dma_start(out=s_b, in_=cout.to_broadcast([P, 1]))
                # c = clip(MIN_C + (MAX_C-MIN_C)*mean(c_pred))
                c_b = sc("c_b", (P, 1))
                V.tensor_scalar(out=c_b, in0=s_b,
                                scalar1=(MAX_C - MIN_C) / B_FULL,
                                scalar2=MIN_C, op0=ALU.mult, op1=ALU.add)
                V.tensor_scalar_min(out=c_b, in0=c_b, scalar1=MAX_C)
                V.tensor_scalar_max(out=c_b, in0=c_b, scalar1=MIN_C)
                negc_b = sc("negc_b", (P, 1))
                V.tensor_scalar_mul(out=negc_b, in0=c_b, scalar1=-1.0)
                twoc_b = sc("twoc_b", (P, 1))
                V.tensor_scalar_mul(out=twoc_b, in0=c_b, scalar1=2.0)
                neg2c_b = sc("neg2c_b", (P, 1))
                V.tensor_scalar_mul(out=neg2c_b, in0=c_b, scalar1=-2.0)
                c2_b = sc("c2_b", (P, 1))
                V.tensor_mul(c2_b, c_b, c_b)

            # ---------- MM2: uT = sigmoid(W2.T @ hT) fp8 DoubleRow ----------
            uT_sb = big.tile([P, KH, BL], bf16, name="uT_sb", tag="big")
            inv_s2 = 1.0 / W2S
            with nc.named_scope("mm2"):
                for mh in range(KH):
                    ps = mm.tile([P, BL], f32, name="ps", tag="mm")
                    w2row = wp.tile([P, KH, P], fp8, name="w2row", tag="w2")
                    nc.sync.dma_start(out=w2row, in_=w2_d[mh])
                    for j in range(KH // 2):
                        nc.tensor.matmul(ps[:, 0:512],
                                         lhsT=w2row[:, 2 * j:2 * j + 2, :],
                                         rhs=hTq_sb[:, 2 * j:2 * j + 2, 0:512],
                                         start=(j == 0), stop=(j == KH // 2 - 1),
                                         perf_mode=PM.DoubleRow)
                        nc.tensor.matmul(ps[:, 512:BL],
                                         lhsT=w2row[:, 2 * j:2 * j + 2, :],
                                         rhs=hTq_sb[:, 2 * j:2 * j + 2, 512:BL],
                                         start=(j == 0), stop=(j == KH // 2 - 1),
                                         perf_mode=PM.DoubleRow)
                    if with_b2:
                        S.activation(uT_sb[:, mh, :], ps, AF.Sigmoid,
                                     bias=b2_sb[:, mh:mh + 1], scale=inv_s2)
                    else:
                        S.activation(uT_sb[:, mh, :], ps, AF.Sigmoid,
                                     scale=inv_s2)
                    uu = scr.tile([P, BL], bf16, name="uu", tag="hh")
                    S.activation(uu, uT_sb[:, mh, :], AF.Square)
                    hu = scr.tile([P, BL], bf16, name="hu", tag="hh")
                    V.tensor_mul(hu, hT_sb[:, mh, :], uT_sb[:, mh, :])
                    if mh == 0:
                        V.tensor_copy(y2a, uu)
                        V.tensor_copy(xya, hu)
                    else:
                        V.tensor_add(y2a, y2a, uu)
                        V.tensor_add(xya, xya, hu)
                # warm-keeper group 1: bridges the PE from mm2's last matmul
                # across the stats-accumulation drain (a >3.4us idle would
                # HAM-throttle the clock for the MMo matmuls).  A real
                # accumulation group so DCE cannot drop it.
                for i in range(14):
                    nc.tensor.matmul(dum_ps, lhsT=ident_sb,
                                     rhs=hT_sb[:, i, 0:384],
                                     start=(i == 0), stop=(i == 13),
                                     skip_group_check=True)
                for blk in range(NB):
                    nc.tensor.matmul(stat_ps[:, 8 + blk:9 + blk],
                                     lhsT=y2a[:, blk * P:(blk + 1) * P],
                                     rhs=onesb, start=True, stop=True,
                                     skip_group_check=True)
                    nc.tensor.matmul(stat_ps[:, 16 + blk:17 + blk],
                                     lhsT=xya[:, blk * P:(blk + 1) * P],
                                     rhs=onesb, start=True, stop=True,
                                     skip_group_check=True)

            # ---------- per-row scalar chain, batch-major [128, 8] ----------
            alpha_b = abp.tile([P, BL], bf16, name="alpha_b")
            beta_b = abp.tile([P, BL], bf16, name="beta_b")
            ab_d = dram.tile([2, BL], bf16, name="ab_d")

            with nc.named_scope("scalars"):
                stats_sb = sc("stats_sb", (P, 24))
                V.tensor_copy(stats_sb, stat_ps[:, 0:24])
                x2 = stats_sb[:, 0:8]
                y2 = stats_sb[:, 8:16]
                xy = stats_sb[:, 16:24]
                w = sc("w")
                V.scalar_tensor_tensor(out=w, in0=xy, scalar=-2.0, in1=y2,
                                       op0=ALU.mult, op1=ALU.add)
                A1 = sc("A1")
                V.tensor_scalar(out=A1, in0=w, scalar1=c_b, scalar2=1.0,
                                op0=ALU.mult, op1=ALU.add)
                A2 = sc("A2")
                V.tensor_scalar(out=A2, in0=x2, scalar1=negc_b, scalar2=1.0,
                                op0=ALU.mult, op1=ALU.add)
                p1 = sc("p1")
                V.tensor_mul(p1, x2, y2)
                den = sc("den")
                V.tensor_scalar(out=den, in0=p1, scalar1=c2_b, scalar2=1.0,
                                op0=ALU.mult, op1=ALU.add)
                V.scalar_tensor_tensor(out=den, in0=xy, scalar=neg2c_b, in1=den,
                                       op0=ALU.mult, op1=ALU.add)
                V.tensor_scalar_add(out=den, in0=den, scalar1=EPS)
                D = sc("D")
                V.reciprocal(D, den)
                nc.tensor.matmul(stat_ps[0:1, 80:88], lhsT=onesf, rhs=D,
                                 start=True, stop=True, skip_group_check=True)
                # ||a||^2 = D^2 (A1^2 x2 - 2 A1 A2 xy + A2^2 y2)
                t1 = sc("t1")
                V.tensor_mul(t1, A1, A1)
                V.tensor_mul(t1, t1, x2)
                t2 = sc("t2")
                V.tensor_mul(t2, A1, A2)
                V.tensor_mul(t2, t2, xy)
                t3 = sc("t3")
                V.tensor_mul(t3, A2, A2)
                V.tensor_mul(t3, t3, y2)
                na2 = sc("na2")
                V.scalar_tensor_tensor(out=na2, in0=t2, scalar=-2.0, in1=t1,
                                       op0=ALU.mult, op1=ALU.add)
                V.tensor_add(na2, na2, t3)
                dsq = sc("dsq")
                V.tensor_mul(dsq, D, D)
                V.tensor_mul(na2, na2, dsq)
                # q = sqrt(c * na2) with one Newton step (ACT sqrt is low precision)
                q2 = sc("q2")
                V.tensor_scalar(out=q2, in0=na2, scalar1=c_b, scalar2=None,
                                op0=ALU.mult)
                q0 = sc("q0")
                S.activation(q0, q2, AF.Sqrt)
                V.tensor_scalar_max(out=q0, in0=q0, scalar1=1e-20)
                r0 = sc("r0")
                V.reciprocal(r0, q0)
                q = sc("q")
                V.tensor_mul(q, q2, r0)
                V.tensor_add(q, q, q0)
                V.tensor_scalar_mul(out=q, in0=q, scalar1=0.5)
                arg = sc("arg")
                V.tensor_scalar_min(out=arg, in0=q, scalar1=1.0 - 1e-5)
                # artanh(arg) = 0.5 ln((1+arg)/(1-arg)); t = tanh(T*artanh)/q
                opp = sc("opp")
                V.tensor_scalar(out=opp, in0=arg, scalar1=-1.0, scalar2=1.0,
                                op0=ALU.mult, op1=ALU.add)
                opn = sc("opn")
                V.tensor_scalar_add(out=opn, in0=arg, scalar1=1.0)
                rr = sc("rr")
                V.reciprocal(rr, opp)
                rat = sc("rat")
                V.tensor_mul(rat, opn, rr)
                lg = sc("lg")
                S.activation(lg, rat, AF.Ln)
                th = sc("th")
                S.activation(th, lg, AF.Tanh, scale=T_CONST * 0.5)
                rq = sc("rq")
                V.reciprocal(rq, q)
                tm = sc("tm")
                V.tensor_mul(tm, th, rq)
                nc.tensor.matmul(stat_ps[0:1, 88:96], lhsT=onesf, rhs=tm,
                                 start=True, stop=True, skip_group_check=True)
                # <h,a> = D (A2 xy - A1 x2)
                s1_ = sc("s1_")
                V.tensor_mul(s1_, A1, x2)
                s2_ = sc("s2_")
                V.tensor_mul(s2_, A2, xy)
                ha = sc("ha")
                V.tensor_sub(ha, s2_, s1_)
                V.tensor_mul(ha, ha, D)
                hm = sc("hm")
                V.tensor_mul(hm, tm, ha)
                tsq = sc("tsq")
                V.tensor_mul(tsq, tm, tm)
                m2 = sc("m2")
                V.tensor_mul(m2, tsq, na2)
                w2s = sc("w2s")
                V.scalar_tensor_tensor(out=w2s, in0=hm, scalar=2.0, in1=m2,
                                       op0=ALU.mult, op1=ALU.add)
                B1 = sc("B1")
                V.tensor_scalar(out=B1, in0=w2s, scalar1=c_b, scalar2=1.0,
                                op0=ALU.mult, op1=ALU.add)
                p2 = sc("p2")
                V.tensor_mul(p2, x2, m2)
                den2 = sc("den2")
                V.tensor_scalar(out=den2, in0=p2, scalar1=c2_b, scalar2=1.0,
                                op0=ALU.mult, op1=ALU.add)
                V.scalar_tensor_tensor(out=den2, in0=hm, scalar=twoc_b, in1=den2,
                                       op0=ALU.mult, op1=ALU.add)
                V.tensor_scalar_add(out=den2, in0=den2, scalar1=EPS)
                D2 = sc("D2")
                V.reciprocal(D2, den2)
                g = sc("g")
                V.tensor_mul(g, A2, tm)
                V.tensor_mul(g, g, D)
                w3 = sc("w3")
                V.tensor_mul(w3, g, A1)
                V.tensor_sub(w3, B1, w3)
                # alpha -> cols 0:8, beta -> cols 8:16 of one bf16 tile; a PE
                # transpose then yields batch-linear rows for a fast DMA
                # bounce (8 contiguous 256B descriptors instead of a 2-byte
                # element scatter).
                ab_bm = sc("ab_bm", (P, 16), bf16)
                V.tensor_mul(ab_bm[:, 0:8], w3, D2)
                w4 = sc("w4")
                V.tensor_mul(w4, g, A2)
                V.tensor_mul(ab_bm[:, 8:16], w4, D2)
                nc.tensor.transpose(abT_ps, ab_bm, ident_sb)
                # warm-keeper taps pinned after the chain end: the scheduler
                # hoists dependency-free matmuls, so these read ab_bm
                for i in range(3):
                    nc.tensor.matmul(stat_ps[:, 32 + 16 * i:48 + 16 * i],
                                     lhsT=ident_sb, rhs=ab_bm,
                                     start=True, stop=True,
                                     skip_group_check=True)
                abT_sb = sc("abT_sb", (16, P), bf16)
                V.tensor_copy(abT_sb, abT_ps)
                nc.sync.dma_start(out=ab_d[0, :].rearrange("(j b) -> j b", j=8),
                                  in_=abT_sb[0:8, :])
                nc.sync.dma_start(out=ab_d[1, :].rearrange("(j b) -> j b", j=8),
                                  in_=abT_sb[8:16, :])
                for hsl in (slice(0, 512), slice(512, BL)):
                    nc.scalar.dma_start(out=alpha_b[:, hsl],
                                        in_=ab_d[0:1, hsl].to_broadcast([P, 512]))
                    nc.sync.dma_start(out=beta_b[:, hsl],
                                      in_=ab_d[1:2, hsl].to_broadcast([P, 512]))
                for i in range(2):
                    nc.tensor.matmul(dum_ps, lhsT=ident_sb,
                                     rhs=(alpha_b[:, 0:384] if i == 0
                                          else beta_b[:, 512:896]),
                                     start=(i == 0), stop=(i == 1),
                                     skip_group_check=True)
                # anchor all warm-keeper matmuls against DCE
                dum_sb = sc("dum_sb", (1, 128), bf16)
                V.tensor_copy(dum_sb, stat_ps[0:1, 24:152])
                dum_d = dram.tile([1, 128], bf16, name="dum_d")
                nc.sync.dma_start(out=dum_d, in_=dum_sb)

        # psum pools (mm, stp) released here

        # ---------- z = alpha*h + beta*u (overwrites uT in place),
        # then out = z @ Wo.  Processed in two batch-column halves so the
        # MMo matmuls of half 0 overlap the DVE z-combine of half 1.
        with ExitStack() as ph2:
            mmo = ph2.enter_context(tc.tile_pool(name="mmo", bufs=8,
                                                 space="PSUM"))
            for bg in range(2):
                csl = slice(bg * 512, (bg + 1) * 512)
                with nc.named_scope(f"zcomb{bg}"):
                    for kh in range(KH):
                        t1z = zzp.tile([P, 512], bf16, name="t1z", tag="zz")
                        V.tensor_mul(t1z, hT_sb[:, kh, csl], alpha_b[:, csl])
                        t2z = zzp.tile([P, 512], bf16, name="t2z", tag="zz")
                        V.tensor_mul(t2z, uT_sb[:, kh, csl], beta_b[:, csl])
                        V.tensor_add(uT_sb[:, kh, csl], t1z, t2z)
                with nc.named_scope(f"mmo{bg}"):
                    pso = [mmo.tile([P, 500], f32, name=f"pso{bg}_{i}",
                                    tag="mmo") for i in range(8)]
                    for kh in range(KH):
                        wot = wop.tile([P, OUT], bf16, name="wot", tag="wo")
                        nc.sync.dma_start(out=wot, in_=wo_d[kh])
                        for i in range(4):
                            b = bg * 4 + i
                            for och in range(2):
                                nc.tensor.matmul(
                                    pso[i * 2 + och],
                                    lhsT=uT_sb[:, kh, b * P:(b + 1) * P],
                                    rhs=wot[:, och * 500:(och + 1) * 500],
                                    start=(kh == 0), stop=(kh == KH - 1))
                    for i in range(4):
                        b = bg * 4 + i
                        for och in range(2):
                            ob = scr.tile([P, 500], f32, name="ob", tag="hh")
                            if och == 0:
                                S.copy(ob, pso[i * 2])
                            else:
                                V.tensor_copy(ob, pso[i * 2 + 1])
                            outq = nc.sync if och == 0 else nc.scalar
                            outq.dma_start(
                                out=out_d[b * P:(b + 1) * P,
                                          och * 500:(och + 1) * 500],
                                in_=ob)

    nc.compile()
    return nc


def _get_nc(with_b1, with_b2):
    for k, v in _nc_cache:
        if k == (with_b1, with_b2):
            return v
    nc = _build(with_b1, with_b2)
    _nc_cache.append(((with_b1, with_b2), nc))
    return nc


def kernel(x, W1, b1, W2, b2, Wo, bo, cp_w1, cp_b1, cp_w2, cp_b2,
           _trace=False, _tmpdir=None):
    x = np.asarray(x, dtype=np.float32)
    with_b1 = bool(np.any(b1))
    with_b2 = bool(np.any(b2))
    nc = _get_nc(with_b1, with_b2)

    # W1 pre-scaled x1024 so fp8 and bf16 parts share one accumulator
    w1s = np.asarray(W1, np.float32) * W2S
    # fp8 rows 0:1024 in DoubleRow pair layout [KH, P, KQ//2, 2, P]
    w1q_t = np.ascontiguousarray(
        np.clip(w1s[:KQ * P], -E4MAX, E4MAX)
        .reshape(KQ, P, KH, P).transpose(2, 1, 0, 3)
    ).astype(E4).reshape(KH, P, KQ // 2, 2, P)
    # bf16 rows 1024:3072 in halves [KH, 2, P, KB//2, P]
    w1_t = np.ascontiguousarray(
        w1s[KQ * P:].reshape(KB, P, KH, P).transpose(2, 1, 0, 3)
        .reshape(KH, P, 2, KB // 2, P).transpose(0, 2, 1, 3, 4)
    ).astype(BF)
    # w2r[mh, p, kh, q] = W2[kh*128+p, mh*128+q], fp8 e4m3 scaled x1024
    w2_t = np.ascontiguousarray(
        np.clip(np.asarray(W2, np.float32) * W2S, -E4MAX, E4MAX)
        .reshape(KH, P, KH, P).transpose(2, 1, 0, 3)
    ).astype(E4)
    wo_t = np.asarray(Wo, np.float32).reshape(KH, P, OUT).astype(BF)
    cpw1_t = np.ascontiguousarray(np.clip(
        np.asarray(cp_w1, np.float32).T * 64.0, -E4MAX, E4MAX)
        .reshape(KI, P, 16)).astype(E4)
    cpw2_t = np.asarray(cp_w2, np.float32).reshape(1, 16).T.astype(BF)
    cpw2_t = np.ascontiguousarray(cpw2_t)
    cpb1_t = np.asarray(cp_b1, np.float32).reshape(16, 1)
    cpb2_t = np.asarray(cp_b2, np.float32).reshape(1, 1)
    b1_t = np.ascontiguousarray(np.asarray(b1, np.float32).reshape(KH, P).T)
    b2_t = np.ascontiguousarray(np.asarray(b2, np.float32).reshape(KH, P).T)

    in_maps = []
    for c in range(N_CORES):
        shard = x[c * BL:(c + 1) * BL]
        shT = np.ascontiguousarray(shard.T)
        xq_c = np.clip(shT[:KQ * P], -E4MAX, E4MAX).reshape(KQ, P, BL).astype(E4)
        xT = shT[KQ * P:].reshape(KB, P, BL).astype(BF)
        m = {"xq": xq_c, "xT": xT, "w1": w1_t, "w1q": w1q_t, "w2": w2_t,
             "wo": wo_t,
             "cpw1": cpw1_t, "cpw2": cpw2_t, "cpb1": cpb1_t, "cpb2": cpb2_t,
             "ident": _IDENT}
        if with_b1:
            m["b1"] = b1_t
        if with_b2:
            m["b2"] = b2_t
        in_maps.append(m)

    kw = {}
    if _trace:
        kw = dict(trace=True, tmpdir=_tmpdir or tempfile.mkdtemp(prefix="cdk_"))
    res = run_bass_kernel_spmd(nc, in_maps, list(range(N_CORES)), **kw)

    out = np.concatenate([res.results[c]["out"] for c in range(N_CORES)], axis=0)
    bo = np.asarray(bo, np.float32)
    if np.any(bo):
        out = out + bo
    if _trace:
        kernel._last_result = res
    return out


# revision 21
# speedup vs baseline: 1.3773x; 1.0119x over previous
"""Trainium2 Bass kernel for nn_ConservativeDynamicCurvatureMLP.

Data-parallel over 8 NeuronCores: the batch (8192) is sharded into 8
local shards of 1024 rows; all weights are replicated. The curvature
scalar c_avg couples the shards through a global mean, handled with a
single-scalar AllReduce.

Math (reference):
    h = tanh(x @ W1 + b1)
    u = sigmoid(h @ W2 + b2)
    c = clip(mean(MIN_C + (MAX_C-MIN_C) * sigmoid(relu(x@cp_w1.T+cp_b1)@cp_w2.T+cp_b2)), MIN_C, MAX_C)
    z = poincare_ball_layer(h, u, c, T)
    out = z @ Wo + bo

The poincare layer collapses algebraically to z = alpha(row)*h + beta(row)*u
where alpha/beta are scalar functions of the row statistics
x2=||h||^2, y2=||u||^2, xy=<h,u> and c.  The NaN fallback is measure-zero
and omitted.

Perf notes (measured on hw):
  - With 8 cores active the PE is power-throttled to ~2.0 GHz (HAM state
    31), so bf16 N=512 matmuls pace at ~263 ns.  The only lever is fewer
    PE cycles: MM2 (h@W2, the largest matmul) runs as fp8-e4m3 DoubleRow
    (256-deep contraction per pass, 2x FLOP rate, measured 216 ns/MM at
    2.4 GHz in isolation).  W2 is host-converted to e4m3 * 1024 (clipped
    to +-240, the TRN e4m3 max); the 1/1024 folds into the sigmoid's
    activation scale.  h is cast bf16->fp8 per row-tile on the DVE.
    MM1 / MMo stay bf16: fp8 there pushes rel-err too close to the 2e-2
    gate (simulated 1.96e-2 vs 1.26e-2 for MM2-only).
  - Row statistics go batch-major directly via tiny stationary-squares
    matmuls (lhsT = accumulated squares block, rhs = ones) instead of a
    feature-major ones-reduction + DRAM bounce, shortening the serial
    stats -> alpha/beta tail.  The per-row scalar chain runs once on
    [128, 8] tiles reading stats straight from SBUF.
"""

import tempfile
from contextlib import ExitStack

import numpy as np
import ml_dtypes

import concourse.bass as bass
import concourse.bacc as bacc
import concourse.mybir as mybir
import concourse.tile as tile
from concourse.bass_utils import run_bass_kernel_spmd

P = 128
N_CORES = 8
B_FULL = 8192
BL = B_FULL // N_CORES          # 1024 rows per core
IN = 3072
HID = 4096
OUT = 1000
KI = IN // P                    # 24
KQ = 8                          # leading ki-groups of MM1 done in fp8
KB = KI - KQ                    # trailing ki-groups in bf16
KH = HID // P                   # 32
NB = BL // P                    # 8 batch tiles
MIN_C = 0.001 * 0.5
MAX_C = 0.001 * 2.0
T_CONST = 0.7
EPS = 1e-7
W2S = 1024.0                    # host-side fp8 scale for W2
E4MAX = 240.0                   # TRN e4m3 saturation

dt = mybir.dt
AF = mybir.ActivationFunctionType
ALU = mybir.AluOpType
PM = mybir.MatmulPerfMode
BF = ml_dtypes.bfloat16
E4 = ml_dtypes.float8_e4m3

_nc_cache = []
_IDENT = np.eye(P, dtype=ml_dtypes.bfloat16)


def _build(with_b1, with_b2):
    nc = bacc.Bacc("TRN2", target_bir_lowering=False, debug=False,
                   num_devices=N_CORES)

    # x features 0:1024 as fp8 (partial-fp8 MM1), 1024:3072 as bf16
    xq_d = nc.dram_tensor("xq", [KQ, P, BL], dt.float8e4, kind="ExternalInput")
    xT_d = nc.dram_tensor("xT", [KB, P, BL], dt.bfloat16, kind="ExternalInput")
    # W1 scaled x1024 throughout (so fp8 and bf16 parts share one psum
    # accumulator); fp8 rows in DoubleRow pair layout, bf16 rows in halves
    w1q_d = nc.dram_tensor("w1q", [KH, P, KQ // 2, 2, P], dt.float8e4,
                           kind="ExternalInput")
    w1_d = nc.dram_tensor("w1", [KH, 2, P, KB // 2, P], dt.bfloat16,
                          kind="ExternalInput")
    # W2 in fp8 (x1024): w2r[mh, p, kh, q] = W2[kh*128+p, mh*128+q]
    w2_d = nc.dram_tensor("w2", [KH, P, KH, P], dt.float8e4, kind="ExternalInput")
    wo_d = nc.dram_tensor("wo", [KH, P, OUT], dt.bfloat16, kind="ExternalInput")
    cpw1_d = nc.dram_tensor("cpw1", [KI, P, 16], dt.float8e4, kind="ExternalInput")
    cpw2_d = nc.dram_tensor("cpw2", [16, 1], dt.bfloat16, kind="ExternalInput")
    cpb1_d = nc.dram_tensor("cpb1", [16, 1], dt.float32, kind="ExternalInput")
    cpb2_d = nc.dram_tensor("cpb2", [1, 1], dt.float32, kind="ExternalInput")
    ident_d = nc.dram_tensor("ident", [P, P], dt.bfloat16, kind="ExternalInput")
    b1_d = nc.dram_tensor("b1", [P, KH], dt.float32, kind="ExternalInput") if with_b1 else None
    b2_d = nc.dram_tensor("b2", [P, KH], dt.float32, kind="ExternalInput") if with_b2 else None
    out_d = nc.dram_tensor("out", [BL, OUT], dt.float32, kind="ExternalOutput")

    f32 = dt.float32
    bf16 = dt.bfloat16
    fp8 = dt.float8e4

    with tile.TileContext(nc) as tc, ExitStack() as ctx:
        const = ctx.enter_context(tc.tile_pool(name="const", bufs=1))
        big = ctx.enter_context(tc.tile_pool(name="big", bufs=1))
        htp = ctx.enter_context(tc.tile_pool(name="htp", bufs=1))
        hqp = ctx.enter_context(tc.tile_pool(name="hqp", bufs=1))
        wp = ctx.enter_context(tc.tile_pool(name="wp", bufs=2))
        wop = ctx.enter_context(tc.tile_pool(name="wop", bufs=2))
        scr = ctx.enter_context(tc.tile_pool(name="scr", bufs=2))
        xqp = ctx.enter_context(tc.tile_pool(name="xqp", bufs=1))
        zzp = ctx.enter_context(tc.tile_pool(name="zzp", bufs=3))
        sacc = ctx.enter_context(tc.tile_pool(name="sacc", bufs=1))
        abp = ctx.enter_context(tc.tile_pool(name="abp", bufs=1))
        scal = ctx.enter_context(tc.tile_pool(name="scal", bufs=1))
        cpp = ctx.enter_context(tc.tile_pool(name="cpp", bufs=1))
        dram = ctx.enter_context(tc.tile_pool(name="dram", bufs=1, space="DRAM"))

        V = nc.vector
        S = nc.scalar

        def sc(name, shape=(P, 8), dtype=f32):
            return scal.tile(list(shape), dtype, name=name, tag=name)

        # ---------- persistent activations (feature-major) ----------
        onesb = const.tile([P, 1], bf16, name="onesb")
        nc.vector.memset(onesb, 1.0)
        onesf = const.tile([P, 1], f32, name="onesf")
        nc.vector.memset(onesf, 1.0)
        xT_sb = big.tile([P, KB, BL], bf16, name="xT_sb", tag="big",
                         padded_shape=[P, KH, BL])
        # first weight half-row issues on the sync queue before anything else;
        # xT streams on the gpsimd + scalar queues in parallel (the load is
        # HBM-bandwidth-bound, ~35us for 6.3MB with all 8 cores pulling)
        w1q0 = wp.tile([P, KQ // 2, 2, P], fp8, name="w1qrow", tag="w1q")
        nc.sync.dma_start(out=w1q0, in_=w1q_d[0])
        xq_sb = xqp.tile([P, KQ, BL], fp8, name="xq_sb")
        queues = [nc.scalar, nc.gpsimd]
        for idx, (a, b) in enumerate(((0, 2), (2, 4), (4, 6), (6, 8))):
            queues[idx % 2].dma_start(
                out=xq_sb[:, a:b, :],
                in_=xq_d[a:b].rearrange("k p b -> p k b"))
        for idx, (a, b) in enumerate(((0, 2), (2, 4), (4, 7), (7, 10),
                                      (10, 13), (13, 16))):
            queues[idx % 2].dma_start(
                out=xT_sb[:, a:b, :],
                in_=xT_d[a:b].rearrange("k p b -> p k b"))
        ident_sb = const.tile([P, P], bf16, name="ident_sb")
        nc.sync.dma_start(out=ident_sb, in_=ident_d[:, :])
        hT_sb = htp.tile([P, KH, BL], bf16, name="hT_sb")
        hTq_sb = hqp.tile([P, KH, BL], fp8, name="hTq_sb")
        if with_b1:
            b1_sb = const.tile([P, KH], f32, name="b1_sb")
            nc.sync.dma_start(out=b1_sb, in_=b1_d[:, :])
        if with_b2:
            b2_sb = const.tile([P, KH], f32, name="b2_sb")
            nc.sync.dma_start(out=b2_sb, in_=b2_d[:, :])

        with ExitStack() as ph1:
            mm = ph1.enter_context(tc.tile_pool(name="mm", bufs=3, space="PSUM"))
            stp = ph1.enter_context(tc.tile_pool(name="stp", bufs=1, space="PSUM"))
            # one bank shared by the batch-major stats (cols 0:8 x2,
            # 8:16 y2, 16:24 xy) and the HAM warm-keeper matmul target
            # (cols 128:512); plus the a/b transpose target bank
            stat_ps = stp.tile([P, 512], f32, name="stat_ps")
            dum_ps = stat_ps[:, 128:512]
            abT_ps = stp.tile([16, P], bf16, name="abT_ps")

            # ---------- MM1: hT = tanh(W1.T @ xT) (bf16) ----------
            x2a = sacc.tile([P, BL], bf16, name="x2a")
            y2a = sacc.tile([P, BL], bf16, name="y2a")
            xya = sacc.tile([P, BL], bf16, name="xya")
            with nc.named_scope("mm1"):
                for mh in range(KH):
                    ps = mm.tile([P, BL], f32, name="ps", tag="mm")
                    if mh == 0:
                        w1q = w1q0
                    else:
                        w1q = wp.tile([P, KQ // 2, 2, P], fp8, name="w1qrow",
                                      tag="w1q")
                        nc.sync.dma_start(out=w1q, in_=w1q_d[mh])
                    w1a = wp.tile([P, KB // 2, P], bf16, name="w1row", tag="w1")
                    nc.sync.dma_start(out=w1a, in_=w1_d[mh, 0])
                    w1b = wp.tile([P, KB // 2, P], bf16, name="w1row", tag="w1")
                    nc.sync.dma_start(out=w1b, in_=w1_d[mh, 1])
                    for hsl in (slice(0, 512), slice(512, BL)):
                        for j in range(KQ // 2):
                            nc.tensor.matmul(
                                ps[:, hsl], lhsT=w1q[:, j, :, :],
                                rhs=xq_sb[:, 2 * j:2 * j + 2, hsl],
                                start=(j == 0), stop=False,
                                perf_mode=PM.DoubleRow,
                                skip_group_check=True)
                        for ki in range(KB):
                            wrow = w1a if ki < KB // 2 else w1b
                            kk = ki % (KB // 2)
                            nc.tensor.matmul(ps[:, hsl], lhsT=wrow[:, kk, :],
                                             rhs=xT_sb[:, ki, hsl],
                                             start=False, stop=(ki == KB - 1),
                                             skip_group_check=True)
                    if with_b1:
                        S.activation(hT_sb[:, mh, :], ps, AF.Tanh,
                                     bias=b1_sb[:, mh:mh + 1], scale=1.0 / W2S)
                    else:
                        S.activation(hT_sb[:, mh, :], ps, AF.Tanh,
                                     scale=1.0 / W2S)
                    # fp8 copy for MM2's DoubleRow rhs
                    V.tensor_copy(hTq_sb[:, mh, :], hT_sb[:, mh, :])
                    hh = scr.tile([P, BL], bf16, name="hh", tag="hh")
                    S.activation(hh, hT_sb[:, mh, :], AF.Square)
                    if mh == 0:
                        V.tensor_copy(x2a, hh)
                    else:
                        V.tensor_add(x2a, x2a, hh)
                # x2 -> batch-major: out[b,0] = sum_p x2a[p, blk*128+b]
                for blk in range(NB):
                    nc.tensor.matmul(stat_ps[:, blk:blk + 1],
                                     lhsT=x2a[:, blk * P:(blk + 1) * P],
                                     rhs=onesb, start=True, stop=True,
                                     skip_group_check=True)

            # ---------- curvature predictor (xT still resident; the
            # AllReduce hides under MM1/MM2) ----------
            with nc.named_scope("cp"):
                cpw1_sb = const.tile([P, KI, 16], fp8, name="cpw1_sb")
                nc.sync.dma_start(out=cpw1_sb,
                                  in_=cpw1_d.rearrange("k p q -> p k q"))
                cpw2_sb = const.tile([16, 1], bf16, name="cpw2_sb")
                nc.sync.dma_start(out=cpw2_sb, in_=cpw2_d[:, :])
                cpb1_sb = const.tile([16, 1], f32, name="cpb1_sb")
                nc.sync.dma_start(out=cpb1_sb, in_=cpb1_d[:, :])
                cpb2_sb = const.tile([1, 1], f32, name="cpb2_sb")
                nc.sync.dma_start(out=cpb2_sb, in_=cpb2_d[:, :])
                cph_sb = cpp.tile([16, BL], fp8, name="cph_sb")
                for ch in range(2):
                    cps = mm.tile([16, 512], f32, name="cps", tag="mm")
                    for ki in range(KI):
                        xr = (xq_sb[:, ki, ch * 512:(ch + 1) * 512] if ki < KQ
                              else xT_sb[:, ki - KQ, ch * 512:(ch + 1) * 512])
                        nc.tensor.matmul(
                            cps, lhsT=cpw1_sb[:, ki, :], rhs=xr,
                            start=(ki == 0), stop=(ki == KI - 1))
                    S.activation(cph_sb[:, ch * 512:(ch + 1) * 512], cps,
                                 AF.Relu, bias=cpb1_sb, scale=1.0 / 64.0)
                sparts = []
                for ch in range(2):
                    c2p = mm.tile([1, 512], f32, name="c2p", tag="mm")
                    nc.tensor.matmul(c2p, lhsT=cpw2_sb,
                                     rhs=cph_sb[:16, ch * 512:(ch + 1) * 512],
                                     start=True, stop=True)
                    cpw = cpp.tile([1, 512], bf16, name="cpw", tag="cpw")
                    spart = cpp.tile([1, 1], f32, name=f"spart{ch}",
                                     tag=f"spart{ch}")
                    S.activation(cpw, c2p, AF.Sigmoid, bias=cpb2_sb,
                                 accum_out=spart)
                    sparts.append(spart)
                s_loc = cpp.tile([1, 1], f32, name="s_loc")
                V.tensor_add(s_loc, sparts[0], sparts[1])
                cin = dram.tile([1, 1], f32, name="cin")
                cout = dram.tile([1, 1], f32, name="cout")
                nc.sync.dma_start(out=cin, in_=s_loc)
                nc.gpsimd.collective_compute(
                    "AllReduce", ALU.add,
                    replica_groups=[list(range(N_CORES))],
                    ins=[cin.opt()], outs=[cout.opt()])
                s_b = sc("s_b", (P, 1))
                nc.gpsimd.dma_start(out=s_b, in_=cout.to_broadcast([P, 1]))
                # c = clip(MIN_C + (MAX_C-MIN_C)*mean(c_pred))
                c_b = sc("c_b", (P, 1))
                V.tensor_scalar(out=c_b, in0=s_b,
                                scalar1=(MAX_C - MIN_C) / B_FULL,
                                scalar2=MIN_C, op0=ALU.mult, op1=ALU.add)
                V.tensor_scalar_min(out=c_b, in0=c_b, scalar1=MAX_C)
                V.tensor_scalar_max(out=c_b, in0=c_b, scalar1=MIN_C)
                negc_b = sc("negc_b", (P, 1))
                V.tensor_scalar_mul(out=negc_b, in0=c_b, scalar1=-1.0)
                twoc_b = sc("twoc_b", (P, 1))
                V.tensor_scalar_mul(out=twoc_b, in0=c_b, scalar1=2.0)
                neg2c_b = sc("neg2c_b", (P, 1))
                V.tensor_scalar_mul(out=neg2c_b, in0=c_b, scalar1=-2.0)
                c2_b = sc("c2_b", (P, 1))
                V.tensor_mul(c2_b, c_b, c_b)

            # ---------- MM2: uT = sigmoid(W2.T @ hT) fp8 DoubleRow ----------
            uT_sb = big.tile([P, KH, BL], bf16, name="uT_sb", tag="big")
            inv_s2 = 1.0 / W2S
            with nc.named_scope("mm2"):
                for mh in range(KH):
                    ps = mm.tile([P, BL], f32, name="ps", tag="mm")
                    w2row = wp.tile([P, KH, P], fp8, name="w2row", tag="w2")
                    nc.sync.dma_start(out=w2row, in_=w2_d[mh])
                    for j in range(KH // 2):
                        nc.tensor.matmul(ps[:, 0:512],
                                         lhsT=w2row[:, 2 * j:2 * j + 2, :],
                                         rhs=hTq_sb[:, 2 * j:2 * j + 2, 0:512],
                                         start=(j == 0), stop=(j == KH // 2 - 1),
                                         perf_mode=PM.DoubleRow)
                        nc.tensor.matmul(ps[:, 512:BL],
                                         lhsT=w2row[:, 2 * j:2 * j + 2, :],
                                         rhs=hTq_sb[:, 2 * j:2 * j + 2, 512:BL],
                                         start=(j == 0), stop=(j == KH // 2 - 1),
                                         perf_mode=PM.DoubleRow)
                    if with_b2:
                        S.activation(uT_sb[:, mh, :], ps, AF.Sigmoid,
                                     bias=b2_sb[:, mh:mh + 1], scale=inv_s2)
                    else:
                        S.activation(uT_sb[:, mh, :], ps, AF.Sigmoid,
                                     scale=inv_s2)
                    uu = scr.tile([P, BL], bf16, name="uu", tag="hh")
                    S.activation(uu, uT_sb[:, mh, :], AF.Square)
                    hu = scr.tile([P, BL], bf16, name="hu", tag="hh")
                    V.tensor_mul(hu, hT_sb[:, mh, :], uT_sb[:, mh, :])
                    if mh == 0:
                        V.tensor_copy(y2a, uu)
                        V.tensor_copy(xya, hu)
                    else:
                        V.tensor_add(y2a, y2a, uu)
                        V.tensor_add(xya, xya, hu)
                # warm-keeper group 1: bridges the PE from mm2's last matmul
                # across the stats-accumulation drain (a >3.4us idle would
                # HAM-throttle the clock for the MMo matmuls).  A real
                # accumulation group so DCE cannot drop it.
                for i in range(14):
                    nc.tensor.matmul(dum_ps, lhsT=ident_sb,
                                     rhs=hT_sb[:, i, 0:384],
                                     start=(i == 0), stop=(i == 13),
                                     skip_group_check=True)
                for blk in range(NB):
                    nc.tensor.matmul(stat_ps[:, 8 + blk:9 + blk],
                                     lhsT=y2a[:, blk * P:(blk + 1) * P],
                                     rhs=onesb, start=True, stop=True,
                                     skip_group_check=True)
                    nc.tensor.matmul(stat_ps[:, 16 + blk:17 + blk],
                                     lhsT=xya[:, blk * P:(blk + 1) * P],
                                     rhs=onesb, start=True, stop=True,
                                     skip_group_check=True)

            # ---------- per-row scalar chain, batch-major [128, 8] ----------
            alpha_b = abp.tile([P, BL], bf16, name="alpha_b", tag="alpha_b")
            beta_b = abp.tile([P, BL], bf16, name="beta_b", tag="beta_b")
            ab_d = dram.tile([2, BL], bf16, name="ab_d")

            with nc.named_scope("scalars"):
                stats_sb = sc("stats_sb", (P, 24))
                V.tensor_copy(stats_sb, stat_ps[:, 0:24])
                x2 = stats_sb[:, 0:8]
                y2 = stats_sb[:, 8:16]
                xy = stats_sb[:, 16:24]
                w = sc("w")
                V.scalar_tensor_tensor(out=w, in0=xy, scalar=-2.0, in1=y2,
                                       op0=ALU.mult, op1=ALU.add)
                A1 = sc("A1")
                V.tensor_scalar(out=A1, in0=w, scalar1=c_b, scalar2=1.0,
                                op0=ALU.mult, op1=ALU.add)
                A2 = sc("A2")
                V.tensor_scalar(out=A2, in0=x2, scalar1=negc_b, scalar2=1.0,
                                op0=ALU.mult, op1=ALU.add)
                p1 = sc("p1")
                V.tensor_mul(p1, x2, y2)
                den = sc("den")
                V.tensor_scalar(out=den, in0=p1, scalar1=c2_b, scalar2=1.0,
                                op0=ALU.mult, op1=ALU.add)
                V.scalar_tensor_tensor(out=den, in0=xy, scalar=neg2c_b, in1=den,
                                       op0=ALU.mult, op1=ALU.add)
                V.tensor_scalar_add(out=den, in0=den, scalar1=EPS)
                D = sc("D")
                V.reciprocal(D, den)
                nc.tensor.matmul(stat_ps[0:1, 80:88], lhsT=onesf, rhs=D,
                                 start=True, stop=True, skip_group_check=True)
                # ||a||^2 = D^2 (A1^2 x2 - 2 A1 A2 xy + A2^2 y2)
                t1 = sc("t1")
                V.tensor_mul(t1, A1, A1)
                V.tensor_mul(t1, t1, x2)
                t2 = sc("t2")
                V.tensor_mul(t2, A1, A2)
                V.tensor_mul(t2, t2, xy)
                t3 = sc("t3")
                V.tensor_mul(t3, A2, A2)
                V.tensor_mul(t3, t3, y2)
                na2 = sc("na2")
                V.scalar_tensor_tensor(out=na2, in0=t2, scalar=-2.0, in1=t1,
                                       op0=ALU.mult, op1=ALU.add)
                V.tensor_add(na2, na2, t3)
                dsq = sc("dsq")
                V.tensor_mul(dsq, D, D)
                V.tensor_mul(na2, na2, dsq)
                # q = sqrt(c * na2) with one Newton step (ACT sqrt is low precision)
                q2 = sc("q2")
                V.tensor_scalar(out=q2, in0=na2, scalar1=c_b, scalar2=None,
                                op0=ALU.mult)
                q0 = sc("q0")
                S.activation(q0, q2, AF.Sqrt)
                V.tensor_scalar_max(out=q0, in0=q0, scalar1=1e-20)
                r0 = sc("r0")
                V.reciprocal(r0, q0)
                q = sc("q")
                V.tensor_mul(q, q2, r0)
                V.tensor_add(q, q, q0)
                V.tensor_scalar_mul(out=q, in0=q, scalar1=0.5)
                arg = sc("arg")
                V.tensor_scalar_min(out=arg, in0=q, scalar1=1.0 - 1e-5)
                # artanh(arg) = 0.5 ln((1+arg)/(1-arg)); t = tanh(T*artanh)/q
                opp = sc("opp")
                V.tensor_scalar(out=opp, in0=arg, scalar1=-1.0, scalar2=1.0,
                                op0=ALU.mult, op1=ALU.add)
                opn = sc("opn")
                V.tensor_scalar_add(out=opn, in0=arg, scalar1=1.0)
                rr = sc("rr")
                V.reciprocal(rr, opp)
                rat = sc("rat")
                V.tensor_mul(rat, opn, rr)
                lg = sc("lg")
                S.activation(lg, rat, AF.Ln)
                th = sc("th")
                S.activation(th, lg, AF.Tanh, scale=T_CONST * 0.5)
                rq = sc("rq")
                V.reciprocal(rq, q)
                tm = sc("tm")
                V.tensor_mul(tm, th, rq)
                nc.tensor.matmul(stat_ps[0:1, 88:96], lhsT=onesf, rhs=tm,
                                 start=True, stop=True, skip_group_check=True)
                # <h,a> = D (A2 xy - A1 x2)
                s1_ = sc("s1_")
                V.tensor_mul(s1_, A1, x2)
                s2_ = sc("s2_")
                V.tensor_mul(s2_, A2, xy)
                ha = sc("ha")
                V.tensor_sub(ha, s2_, s1_)
                V.tensor_mul(ha, ha, D)
                hm = sc("hm")
                V.tensor_mul(hm, tm, ha)
                tsq = sc("tsq")
                V.tensor_mul(tsq, tm, tm)
                m2 = sc("m2")
                V.tensor_mul(m2, tsq, na2)
                w2s = sc("w2s")
                V.scalar_tensor_tensor(out=w2s, in0=hm, scalar=2.0, in1=m2,
                                       op0=ALU.mult, op1=ALU.add)
                B1 = sc("B1")
                V.tensor_scalar(out=B1, in0=w2s, scalar1=c_b, scalar2=1.0,
                                op0=ALU.mult, op1=ALU.add)
                p2 = sc("p2")
                V.tensor_mul(p2, x2, m2)
                den2 = sc("den2")
                V.tensor_scalar(out=den2, in0=p2, scalar1=c2_b, scalar2=1.0,
                                op0=ALU.mult, op1=ALU.add)
                V.scalar_tensor_tensor(out=den2, in0=hm, scalar=twoc_b, in1=den2,
                                       op0=ALU.mult, op1=ALU.add)
                V.tensor_scalar_add(out=den2, in0=den2, scalar1=EPS)
                D2 = sc("D2")
                V.reciprocal(D2, den2)
                g = sc("g")
                V.tensor_mul(g, A2, tm)
                V.tensor_mul(g, g, D)
                w3 = sc("w3")
                V.tensor_mul(w3, g, A1)
                V.tensor_sub(w3, B1, w3)
                # alpha -> cols 0:8, beta -> cols 8:16 of one bf16 tile; a PE
                # transpose then yields batch-linear rows for a fast DMA
                # bounce (8 contiguous 256B descriptors instead of a 2-byte
                # element scatter).
                ab_bm = sc("ab_bm", (P, 16), bf16)
                V.tensor_mul(ab_bm[:, 0:8], w3, D2)
                w4 = sc("w4")
                V.tensor_mul(w4, g, A2)
                V.tensor_mul(ab_bm[:, 8:16], w4, D2)
                nc.tensor.transpose(abT_ps, ab_bm, ident_sb)
                # warm-keeper taps pinned after the chain end: the scheduler
                # hoists dependency-free matmuls, so these read ab_bm
                for i in range(3):
                    nc.tensor.matmul(stat_ps[:, 32 + 16 * i:48 + 16 * i],
                                     lhsT=ident_sb, rhs=ab_bm,
                                     start=True, stop=True,
                                     skip_group_check=True)
                abT_sb = sc("abT_sb", (16, P), bf16)
                V.tensor_copy(abT_sb, abT_ps)
                nc.sync.dma_start(out=ab_d[0, :].rearrange("(j b) -> j b", j=8),
                                  in_=abT_sb[0:8, :])
                nc.sync.dma_start(out=ab_d[1, :].rearrange("(j b) -> j b", j=8),
                                  in_=abT_sb[8:16, :])
                for hsl in (slice(0, 512), slice(512, BL)):
                    nc.scalar.dma_start(out=alpha_b[:, hsl],
                                        in_=ab_d[0:1, hsl].to_broadcast([P, 512]))
                    nc.sync.dma_start(out=beta_b[:, hsl],
                                      in_=ab_d[1:2, hsl].to_broadcast([P, 512]))
                for i in range(2):
                    nc.tensor.matmul(dum_ps, lhsT=ident_sb,
                                     rhs=(alpha_b[:, 0:384] if i == 0
                                          else beta_b[:, 512:896]),
                                     start=(i == 0), stop=(i == 1),
                                     skip_group_check=True)
                # anchor all warm-keeper matmuls against DCE
                dum_sb = sc("dum_sb", (1, 128), bf16)
                V.tensor_copy(dum_sb, stat_ps[0:1, 24:152])
                dum_d = dram.tile([1, 128], bf16, name="dum_d")
                nc.gpsimd.dma_start(out=dum_d, in_=dum_sb)

        # psum pools (mm, stp) released here

        # ---------- z = alpha*h + beta*u (overwrites uT in place),
        # then out = z @ Wo.  Processed in two batch-column halves so the
        # MMo matmuls of half 0 overlap the DVE z-combine of half 1.
        with ExitStack() as ph2:
            mmo = ph2.enter_context(tc.tile_pool(name="mmo", bufs=8,
                                                 space="PSUM"))
            for bg in range(2):
                csl = slice(bg * 512, (bg + 1) * 512)
                with nc.named_scope(f"zcomb{bg}"):
                    for kh in range(KH):
                        t1z = zzp.tile([P, 512], bf16, name="t1z", tag="zz")
                        V.tensor_mul(t1z, hT_sb[:, kh, csl], alpha_b[:, csl])
                        t2z = zzp.tile([P, 512], bf16, name="t2z", tag="zz")
                        V.tensor_mul(t2z, uT_sb[:, kh, csl], beta_b[:, csl])
                        V.tensor_add(uT_sb[:, kh, csl], t1z, t2z)
                with nc.named_scope(f"mmo{bg}"):
                    pso = [mmo.tile([P, 500], f32, name=f"pso{bg}_{i}",
                                    tag="mmo") for i in range(8)]
                    for kh in range(KH):
                        wot = wop.tile([P, OUT], bf16, name="wot", tag="wo")
                        nc.sync.dma_start(out=wot, in_=wo_d[kh])
                        for i in range(4):
                            b = bg * 4 + i
                            for och in range(2):
                                nc.tensor.matmul(
                                    pso[i * 2 + och],
                                    lhsT=uT_sb[:, kh, b * P:(b + 1) * P],
                                    rhs=wot[:, och * 500:(och + 1) * 500],
                                    start=(kh == 0), stop=(kh == KH - 1))
                    for i in range(4):
                        b = bg * 4 + i
                        for och in range(2):
                            # stage drains across the scr ring and the (now
                            # dead) alpha/beta tiles for 4-deep DMA pipelining
                            slot = (i * 2 + och) % 3
                            if slot == 0:
                                ob = scr.tile([P, 500], f32, name="ob",
                                              tag="hh")
                            elif slot == 1:
                                ob = abp.tile([P, 500], f32, name="ob",
                                              tag="alpha_b")
                            else:
                                ob = abp.tile([P, 500], f32, name="ob",
                                              tag="beta_b")
                            if och == 0:
                                S.copy(ob, pso[i * 2])
                            else:
                                V.tensor_copy(ob, pso[i * 2 + 1])
                            outq = nc.sync if och == 0 else nc.scalar
                            outq.dma_start(
                                out=out_d[b * P:(b + 1) * P,
                                          och * 500:(och + 1) * 500],
                                in_=ob)

    nc.compile()
    return nc


def _get_nc(with_b1, with_b2):
    for k, v in _nc_cache:
        if k == (with_b1, with_b2):
            return v
    nc = _build(with_b1, with_b2)
    _nc_cache.append(((with_b1, with_b2), nc))
    return nc


def kernel(x, W1, b1, W2, b2, Wo, bo, cp_w1, cp_b1, cp_w2, cp_b2,
           _trace=False, _tmpdir=None):
    x = np.asarray(x, dtype=np.float32)
    with_b1 = bool(np.any(b1))
    with_b2 = bool(np.any(b2))
    nc = _get_nc(with_b1, with_b2)

    # W1 pre-scaled x1024 so fp8 and bf16 parts share one accumulator
    w1s = np.asarray(W1, np.float32) * W2S
    # fp8 rows 0:1024 in DoubleRow pair layout [KH, P, KQ//2, 2, P]
    w1q_t = np.ascontiguousarray(
        np.clip(w1s[:KQ * P], -E4MAX, E4MAX)
        .reshape(KQ, P, KH, P).transpose(2, 1, 0, 3)
    ).astype(E4).reshape(KH, P, KQ // 2, 2, P)
    # bf16 rows 1024:3072 in halves [KH, 2, P, KB//2, P]
    w1_t = np.ascontiguousarray(
        w1s[KQ * P:].reshape(KB, P, KH, P).transpose(2, 1, 0, 3)
        .reshape(KH, P, 2, KB // 2, P).transpose(0, 2, 1, 3, 4)
    ).astype(BF)
    # w2r[mh, p, kh, q] = W2[kh*128+p, mh*128+q], fp8 e4m3 scaled x1024
    w2_t = np.ascontiguousarray(
        np.clip(np.asarray(W2, np.float32) * W2S, -E4MAX, E4MAX)
        .reshape(KH, P, KH, P).transpose(2, 1, 0, 3)
    ).astype(E4)
    wo_t = np.asarray(Wo, np.float32).reshape(KH, P, OUT).astype(BF)
    cpw1_t = np.ascontiguousarray(np.clip(
        np.asarray(cp_w1, np.float32).T * 64.0, -E4MAX, E4MAX)
        .reshape(KI, P, 16)).astype(E4)
    cpw2_t = np.asarray(cp_w2, np.float32).reshape(1, 16).T.astype(BF)
    cpw2_t = np.ascontiguousarray(cpw2_t)
    cpb1_t = np.asarray(cp_b1, np.float32).reshape(16, 1)
    cpb2_t = np.asarray(cp_b2, np.float32).reshape(1, 1)
    b1_t = np.ascontiguousarray(np.asarray(b1, np.float32).reshape(KH, P).T)
    b2_t = np.ascontiguousarray(np.asarray(b2, np.float32).reshape(KH, P).T)

    in_maps = []
    for c in range(N_CORES):
        shard = x[c * BL:(c + 1) * BL]
        shT = np.ascontiguousarray(shard.T)
        xq_c = np.clip(shT[:KQ * P], -E4MAX, E4MAX).reshape(KQ, P, BL).astype(E4)
        xT = shT[KQ * P:].reshape(KB, P, BL).astype(BF)
        m = {"xq": xq_c, "xT": xT, "w1": w1_t, "w1q": w1q_t, "w2": w2_t,
             "wo": wo_t,
             "cpw1": cpw1_t, "cpw2": cpw2_t, "cpb1": cpb1_t, "cpb2": cpb2_t,
             "ident": _IDENT}
        if with_b1:
            m["b1"] = b1_t
        if with_b2:
            m["b2"] = b2_t
        in_maps.append(m)

    kw = {}
    if _trace:
        kw = dict(trace=True, tmpdir=_tmpdir or tempfile.mkdtemp(prefix="cdk_"))
    res = run_bass_kernel_spmd(nc, in_maps, list(range(N_CORES)), **kw)

    out = np.concatenate([res.results[c]["out"] for c in range(N_CORES)], axis=0)
    bo = np.asarray(bo, np.float32)
    if np.any(bo):
        out = out + bo
    if _trace:
        kernel._last_result = res
    return out


# revision 23
# speedup vs baseline: 1.4309x; 1.0390x over previous
"""Trainium2 Bass kernel for nn_ConservativeDynamicCurvatureMLP.

Data-parallel over 8 NeuronCores: the batch (8192) is sharded into 8
local shards of 1024 rows; all weights are replicated. The curvature
scalar c_avg couples the shards through a global mean, handled with a
single-scalar AllReduce.

Math (reference):
    h = tanh(x @ W1 + b1)
    u = sigmoid(h @ W2 + b2)
    c = clip(mean(MIN_C + (MAX_C-MIN_C) * sigmoid(relu(x@cp_w1.T+cp_b1)@cp_w2.T+cp_b2)), MIN_C, MAX_C)
    z = poincare_ball_layer(h, u, c, T)
    out = z @ Wo + bo

The poincare layer collapses algebraically to z = alpha(row)*h + beta(row)*u
where alpha/beta are scalar functions of the row statistics
x2=||h||^2, y2=||u||^2, xy=<h,u> and c.  The NaN fallback is measure-zero
and omitted.

Perf notes (measured on hw):
  - With 8 cores active the PE is power-throttled to ~2.0 GHz (HAM state
    31), so bf16 N=512 matmuls pace at ~263 ns.  The only lever is fewer
    PE cycles: MM2 (h@W2, the largest matmul) runs as fp8-e4m3 DoubleRow
    (256-deep contraction per pass, 2x FLOP rate, measured 216 ns/MM at
    2.4 GHz in isolation).  W2 is host-converted to e4m3 * 1024 (clipped
    to +-240, the TRN e4m3 max); the 1/1024 folds into the sigmoid's
    activation scale.  h is cast bf16->fp8 per row-tile on the DVE.
    MM1 / MMo stay bf16: fp8 there pushes rel-err too close to the 2e-2
    gate (simulated 1.96e-2 vs 1.26e-2 for MM2-only).
  - Row statistics go batch-major directly via tiny stationary-squares
    matmuls (lhsT = accumulated squares block, rhs = ones) instead of a
    feature-major ones-reduction + DRAM bounce, shortening the serial
    stats -> alpha/beta tail.  The per-row scalar chain runs once on
    [128, 8] tiles reading stats straight from SBUF.
"""

import tempfile
from contextlib import ExitStack

import numpy as np
import ml_dtypes

import concourse.bass as bass
import concourse.bacc as bacc
import concourse.mybir as mybir
import concourse.tile as tile
from concourse.bass_utils import run_bass_kernel_spmd

P = 128
N_CORES = 8
B_FULL = 8192
BL = B_FULL // N_CORES          # 1024 rows per core
IN = 3072
HID = 4096
OUT = 1000
KI = IN // P                    # 24
KQ = 12                         # leading ki-groups of MM1 done in fp8
KB = KI - KQ                    # trailing ki-groups in bf16
KH = HID // P                   # 32
NB = BL // P                    # 8 batch tiles
MIN_C = 0.001 * 0.5
MAX_C = 0.001 * 2.0
T_CONST = 0.7
EPS = 1e-7
W2S = 1024.0                    # host-side fp8 scale for W2
E4MAX = 240.0                   # TRN e4m3 saturation

dt = mybir.dt
AF = mybir.ActivationFunctionType
ALU = mybir.AluOpType
PM = mybir.MatmulPerfMode
BF = ml_dtypes.bfloat16
E4 = ml_dtypes.float8_e4m3

_nc_cache = []
_IDENT = np.eye(P, dtype=ml_dtypes.bfloat16)


def _build(with_b1, with_b2):
    nc = bacc.Bacc("TRN2", target_bir_lowering=False, debug=False,
                   num_devices=N_CORES)

    # x features 0:1024 as fp8 (partial-fp8 MM1), 1024:3072 as bf16
    xq_d = nc.dram_tensor("xq", [KQ, P, BL], dt.float8e4, kind="ExternalInput")
    xT_d = nc.dram_tensor("xT", [KB, P, BL], dt.bfloat16, kind="ExternalInput")
    # W1 scaled x1024 throughout (so fp8 and bf16 parts share one psum
    # accumulator); fp8 rows in DoubleRow pair layout, bf16 rows in halves
    w1q_d = nc.dram_tensor("w1q", [KH, P, KQ // 2, 2, P], dt.float8e4,
                           kind="ExternalInput")
    w1_d = nc.dram_tensor("w1", [KH, 2, P, KB // 2, P], dt.bfloat16,
                          kind="ExternalInput")
    # W2 in fp8 (x1024): w2r[mh, p, kh, q] = W2[kh*128+p, mh*128+q]
    w2_d = nc.dram_tensor("w2", [KH, P, KH, P], dt.float8e4, kind="ExternalInput")
    wo_d = nc.dram_tensor("wo", [KH, P, OUT], dt.bfloat16, kind="ExternalInput")
    cpw1_d = nc.dram_tensor("cpw1", [KI, P, 16], dt.float8e4, kind="ExternalInput")
    cpw2_d = nc.dram_tensor("cpw2", [16, 1], dt.bfloat16, kind="ExternalInput")
    cpb1_d = nc.dram_tensor("cpb1", [16, 1], dt.float32, kind="ExternalInput")
    cpb2_d = nc.dram_tensor("cpb2", [1, 1], dt.float32, kind="ExternalInput")
    ident_d = nc.dram_tensor("ident", [P, P], dt.bfloat16, kind="ExternalInput")
    b1_d = nc.dram_tensor("b1", [P, KH], dt.float32, kind="ExternalInput") if with_b1 else None
    b2_d = nc.dram_tensor("b2", [P, KH], dt.float32, kind="ExternalInput") if with_b2 else None
    out_d = nc.dram_tensor("out", [BL, OUT], dt.float32, kind="ExternalOutput")

    f32 = dt.float32
    bf16 = dt.bfloat16
    fp8 = dt.float8e4

    with tile.TileContext(nc) as tc, ExitStack() as ctx:
        const = ctx.enter_context(tc.tile_pool(name="const", bufs=1))
        big = ctx.enter_context(tc.tile_pool(name="big", bufs=1))
        htp = ctx.enter_context(tc.tile_pool(name="htp", bufs=1))
        hqp = ctx.enter_context(tc.tile_pool(name="hqp", bufs=1))
        wp = ctx.enter_context(tc.tile_pool(name="wp", bufs=2))
        wop = ctx.enter_context(tc.tile_pool(name="wop", bufs=2))
        scr = ctx.enter_context(tc.tile_pool(name="scr", bufs=2))
        zzp = ctx.enter_context(tc.tile_pool(name="zzp", bufs=3))
        sacc = ctx.enter_context(tc.tile_pool(name="sacc", bufs=1))
        abp = ctx.enter_context(tc.tile_pool(name="abp", bufs=1))
        scal = ctx.enter_context(tc.tile_pool(name="scal", bufs=1))
        cpp = ctx.enter_context(tc.tile_pool(name="cpp", bufs=1))
        dram = ctx.enter_context(tc.tile_pool(name="dram", bufs=1, space="DRAM"))

        V = nc.vector
        S = nc.scalar

        def sc(name, shape=(P, 8), dtype=f32):
            return scal.tile(list(shape), dtype, name=name, tag=name)

        # ---------- persistent activations (feature-major) ----------
        onesb = const.tile([P, 1], bf16, name="onesb")
        nc.vector.memset(onesb, 1.0)
        onesf = const.tile([P, 1], f32, name="onesf")
        nc.vector.memset(onesf, 1.0)
        # one 64KB slot holds bf16 x (ki 0:KB), fp8 x (as bitcast bytes in
        # the tail), and is later recycled as uT
        xall = big.tile([P, KH, BL], bf16, name="xall", tag="big")
        xT_sb = xall[:, 0:KB, :]
        # first weight half-row issues on the sync queue before anything else;
        # xT streams on the gpsimd + scalar queues in parallel (the load is
        # HBM-bandwidth-bound, ~35us for 6.3MB with all 8 cores pulling)
        w1q0 = wp.tile([P, KQ // 2, 2, P], fp8, name="w1qrow", tag="w1q")
        nc.sync.dma_start(out=w1q0, in_=w1q_d[0])
        xq_sb = (xall[:, KB:KB + KQ // 2, :].bitcast(fp8)
                 .rearrange("p a (t b) -> p (a t) b", t=2))
        queues = [nc.scalar, nc.gpsimd]
        for idx, (a, b) in enumerate(((0, 3), (3, 6), (6, 9), (9, 12))):
            queues[idx % 2].dma_start(
                out=xq_sb[:, a:b, :],
                in_=xq_d[a:b].rearrange("k p b -> p k b"))
        for idx, (a, b) in enumerate(((0, 2), (2, 4), (4, 6), (6, 8),
                                      (8, 10), (10, 12))):
            queues[idx % 2].dma_start(
                out=xT_sb[:, a:b, :],
                in_=xT_d[a:b].rearrange("k p b -> p k b"))
        ident_sb = const.tile([P, P], bf16, name="ident_sb")
        nc.sync.dma_start(out=ident_sb, in_=ident_d[:, :])
        hT_sb = htp.tile([P, KH, BL], bf16, name="hT_sb")
        hTq_sb = hqp.tile([P, KH, BL], fp8, name="hTq_sb")
        if with_b1:
            b1_sb = const.tile([P, KH], f32, name="b1_sb")
            nc.sync.dma_start(out=b1_sb, in_=b1_d[:, :])
        if with_b2:
            b2_sb = const.tile([P, KH], f32, name="b2_sb")
            nc.sync.dma_start(out=b2_sb, in_=b2_d[:, :])

        with ExitStack() as ph1:
            mm = ph1.enter_context(tc.tile_pool(name="mm", bufs=3, space="PSUM"))
            stp = ph1.enter_context(tc.tile_pool(name="stp", bufs=1, space="PSUM"))
            # one bank shared by the batch-major stats (cols 0:8 x2,
            # 8:16 y2, 16:24 xy) and the HAM warm-keeper matmul target
            # (cols 128:512); plus the a/b transpose target bank
            stat_ps = stp.tile([P, 512], f32, name="stat_ps")
            dum_ps = stat_ps[:, 128:512]
            abT_ps = stp.tile([16, P], bf16, name="abT_ps")

            # ---------- MM1: hT = tanh(W1.T @ xT) (bf16) ----------
            x2a = sacc.tile([P, BL], bf16, name="x2a")
            y2a = sacc.tile([P, BL], bf16, name="y2a")
            xya = sacc.tile([P, BL], bf16, name="xya")
            with nc.named_scope("mm1"):
                for mh in range(KH):
                    ps = mm.tile([P, BL], f32, name="ps", tag="mm")
                    if mh == 0:
                        w1q = w1q0
                    else:
                        w1q = wp.tile([P, KQ // 2, 2, P], fp8, name="w1qrow",
                                      tag="w1q")
                        nc.sync.dma_start(out=w1q, in_=w1q_d[mh])
                    w1a = wp.tile([P, KB // 2, P], bf16, name="w1row", tag="w1")
                    nc.sync.dma_start(out=w1a, in_=w1_d[mh, 0])
                    w1b = wp.tile([P, KB // 2, P], bf16, name="w1row", tag="w1")
                    nc.sync.dma_start(out=w1b, in_=w1_d[mh, 1])
                    for hsl in (slice(0, 512), slice(512, BL)):
                        for j in range(KQ // 2):
                            nc.tensor.matmul(
                                ps[:, hsl], lhsT=w1q[:, j, :, :],
                                rhs=xq_sb[:, 2 * j:2 * j + 2, hsl],
                                start=(j == 0), stop=False,
                                perf_mode=PM.DoubleRow,
                                skip_group_check=True)
                        for ki in range(KB):
                            wrow = w1a if ki < KB // 2 else w1b
                            kk = ki % (KB // 2)
                            nc.tensor.matmul(ps[:, hsl], lhsT=wrow[:, kk, :],
                                             rhs=xT_sb[:, ki, hsl],
                                             start=False, stop=(ki == KB - 1),
                                             skip_group_check=True)
                    if with_b1:
                        S.activation(hT_sb[:, mh, :], ps, AF.Tanh,
                                     bias=b1_sb[:, mh:mh + 1], scale=1.0 / W2S)
                    else:
                        S.activation(hT_sb[:, mh, :], ps, AF.Tanh,
                                     scale=1.0 / W2S)
                    # fp8 copy for MM2's DoubleRow rhs
                    V.tensor_copy(hTq_sb[:, mh, :], hT_sb[:, mh, :])
                    hh = scr.tile([P, BL], bf16, name="hh", tag="hh")
                    S.activation(hh, hT_sb[:, mh, :], AF.Square)
                    if mh == 0:
                        V.tensor_copy(x2a, hh)
                    else:
                        V.tensor_add(x2a, x2a, hh)
                # x2 -> batch-major: out[b,0] = sum_p x2a[p, blk*128+b]
                for blk in range(NB):
                    nc.tensor.matmul(stat_ps[:, blk:blk + 1],
                                     lhsT=x2a[:, blk * P:(blk + 1) * P],
                                     rhs=onesb, start=True, stop=True,
                                     skip_group_check=True)

            # ---------- curvature predictor (xT still resident; the
            # AllReduce hides under MM1/MM2) ----------
            with nc.named_scope("cp"):
                cpw1_sb = const.tile([P, KI, 16], fp8, name="cpw1_sb")
                nc.sync.dma_start(out=cpw1_sb,
                                  in_=cpw1_d.rearrange("k p q -> p k q"))
                cpw2_sb = const.tile([16, 1], bf16, name="cpw2_sb")
                nc.sync.dma_start(out=cpw2_sb, in_=cpw2_d[:, :])
                cpb1_sb = const.tile([16, 1], f32, name="cpb1_sb")
                nc.sync.dma_start(out=cpb1_sb, in_=cpb1_d[:, :])
                cpb2_sb = const.tile([1, 1], f32, name="cpb2_sb")
                nc.sync.dma_start(out=cpb2_sb, in_=cpb2_d[:, :])
                cph_sb = cpp.tile([16, BL], fp8, name="cph_sb")
                for ch in range(2):
                    cps = mm.tile([16, 512], f32, name="cps", tag="mm")
                    for ki in range(KI):
                        xr = (xq_sb[:, ki, ch * 512:(ch + 1) * 512] if ki < KQ
                              else xT_sb[:, ki - KQ, ch * 512:(ch + 1) * 512])
                        nc.tensor.matmul(
                            cps, lhsT=cpw1_sb[:, ki, :], rhs=xr,
                            start=(ki == 0), stop=(ki == KI - 1))
                    S.activation(cph_sb[:, ch * 512:(ch + 1) * 512], cps,
                                 AF.Relu, bias=cpb1_sb, scale=1.0 / 64.0)
                sparts = []
                for ch in range(2):
                    c2p = mm.tile([1, 512], f32, name="c2p", tag="mm")
                    nc.tensor.matmul(c2p, lhsT=cpw2_sb,
                                     rhs=cph_sb[:16, ch * 512:(ch + 1) * 512],
                                     start=True, stop=True)
                    cpw = cpp.tile([1, 512], bf16, name="cpw", tag="cpw")
                    spart = cpp.tile([1, 1], f32, name=f"spart{ch}",
                                     tag=f"spart{ch}")
                    S.activation(cpw, c2p, AF.Sigmoid, bias=cpb2_sb,
                                 accum_out=spart)
                    sparts.append(spart)
                s_loc = cpp.tile([1, 1], f32, name="s_loc")
                V.tensor_add(s_loc, sparts[0], sparts[1])
                cin = dram.tile([1, 1], f32, name="cin")
                cout = dram.tile([1, 1], f32, name="cout")
                nc.sync.dma_start(out=cin, in_=s_loc)
                nc.gpsimd.collective_compute(
                    "AllReduce", ALU.add,
                    replica_groups=[list(range(N_CORES))],
                    ins=[cin.opt()], outs=[cout.opt()])
                s_b = sc("s_b", (P, 1))
                nc.gpsimd.dma_start(out=s_b, in_=cout.to_broadcast([P, 1]))
                # c = clip(MIN_C + (MAX_C-MIN_C)*mean(c_pred))
                c_b = sc("c_b", (P, 1))
                V.tensor_scalar(out=c_b, in0=s_b,
                                scalar1=(MAX_C - MIN_C) / B_FULL,
                                scalar2=MIN_C, op0=ALU.mult, op1=ALU.add)
                V.tensor_scalar_min(out=c_b, in0=c_b, scalar1=MAX_C)
                V.tensor_scalar_max(out=c_b, in0=c_b, scalar1=MIN_C)
                negc_b = sc("negc_b", (P, 1))
                V.tensor_scalar_mul(out=negc_b, in0=c_b, scalar1=-1.0)
                twoc_b = sc("twoc_b", (P, 1))
                V.tensor_scalar_mul(out=twoc_b, in0=c_b, scalar1=2.0)
                neg2c_b = sc("neg2c_b", (P, 1))
                V.tensor_scalar_mul(out=neg2c_b, in0=c_b, scalar1=-2.0)
                c2_b = sc("c2_b", (P, 1))
                V.tensor_mul(c2_b, c_b, c_b)

            # ---------- MM2: uT = sigmoid(W2.T @ hT) fp8 DoubleRow ----------
            uT_sb = big.tile([P, KH, BL], bf16, name="uT_sb", tag="big")
            inv_s2 = 1.0 / W2S
            with nc.named_scope("mm2"):
                for mh in range(KH):
                    ps = mm.tile([P, BL], f32, name="ps", tag="mm")
                    w2row = wp.tile([P, KH, P], fp8, name="w2row", tag="w2")
                    nc.sync.dma_start(out=w2row, in_=w2_d[mh])
                    for j in range(KH // 2):
                        nc.tensor.matmul(ps[:, 0:512],
                                         lhsT=w2row[:, 2 * j:2 * j + 2, :],
                                         rhs=hTq_sb[:, 2 * j:2 * j + 2, 0:512],
                                         start=(j == 0), stop=(j == KH // 2 - 1),
                                         perf_mode=PM.DoubleRow)
                        nc.tensor.matmul(ps[:, 512:BL],
                                         lhsT=w2row[:, 2 * j:2 * j + 2, :],
                                         rhs=hTq_sb[:, 2 * j:2 * j + 2, 512:BL],
                                         start=(j == 0), stop=(j == KH // 2 - 1),
                                         perf_mode=PM.DoubleRow)
                    if with_b2:
                        S.activation(uT_sb[:, mh, :], ps, AF.Sigmoid,
                                     bias=b2_sb[:, mh:mh + 1], scale=inv_s2)
                    else:
                        S.activation(uT_sb[:, mh, :], ps, AF.Sigmoid,
                                     scale=inv_s2)
                    uu = scr.tile([P, BL], bf16, name="uu", tag="hh")
                    S.activation(uu, uT_sb[:, mh, :], AF.Square)
                    hu = scr.tile([P, BL], bf16, name="hu", tag="hh")
                    V.tensor_mul(hu, hT_sb[:, mh, :], uT_sb[:, mh, :])
                    if mh == 0:
                        V.tensor_copy(y2a, uu)
                        V.tensor_copy(xya, hu)
                    else:
                        V.tensor_add(y2a, y2a, uu)
                        V.tensor_add(xya, xya, hu)
                # warm-keeper group 1: bridges the PE from mm2's last matmul
                # across the stats-accumulation drain (a >3.4us idle would
                # HAM-throttle the clock for the MMo matmuls).  A real
                # accumulation group so DCE cannot drop it.
                for i in range(14):
                    nc.tensor.matmul(dum_ps, lhsT=ident_sb,
                                     rhs=hT_sb[:, i, 0:384],
                                     start=(i == 0), stop=(i == 13),
                                     skip_group_check=True)
                for blk in range(NB):
                    nc.tensor.matmul(stat_ps[:, 8 + blk:9 + blk],
                                     lhsT=y2a[:, blk * P:(blk + 1) * P],
                                     rhs=onesb, start=True, stop=True,
                                     skip_group_check=True)
                    nc.tensor.matmul(stat_ps[:, 16 + blk:17 + blk],
                                     lhsT=xya[:, blk * P:(blk + 1) * P],
                                     rhs=onesb, start=True, stop=True,
                                     skip_group_check=True)

            # ---------- per-row scalar chain, batch-major [128, 8] ----------
            alpha_b = abp.tile([P, BL], bf16, name="alpha_b", tag="alpha_b")
            beta_b = abp.tile([P, BL], bf16, name="beta_b", tag="beta_b")
            ab_d = dram.tile([2, BL], bf16, name="ab_d")

            with nc.named_scope("scalars"):
                stats_sb = sc("stats_sb", (P, 24))
                V.tensor_copy(stats_sb, stat_ps[:, 0:24])
                x2 = stats_sb[:, 0:8]
                y2 = stats_sb[:, 8:16]
                xy = stats_sb[:, 16:24]
                w = sc("w")
                V.scalar_tensor_tensor(out=w, in0=xy, scalar=-2.0, in1=y2,
                                       op0=ALU.mult, op1=ALU.add)
                A1 = sc("A1")
                V.tensor_scalar(out=A1, in0=w, scalar1=c_b, scalar2=1.0,
                                op0=ALU.mult, op1=ALU.add)
                A2 = sc("A2")
                V.tensor_scalar(out=A2, in0=x2, scalar1=negc_b, scalar2=1.0,
                                op0=ALU.mult, op1=ALU.add)
                p1 = sc("p1")
                V.tensor_mul(p1, x2, y2)
                den = sc("den")
                V.tensor_scalar(out=den, in0=p1, scalar1=c2_b, scalar2=1.0,
                                op0=ALU.mult, op1=ALU.add)
                V.scalar_tensor_tensor(out=den, in0=xy, scalar=neg2c_b, in1=den,
                                       op0=ALU.mult, op1=ALU.add)
                V.tensor_scalar_add(out=den, in0=den, scalar1=EPS)
                D = sc("D")
                V.reciprocal(D, den)
                nc.tensor.matmul(stat_ps[0:1, 80:88], lhsT=onesf, rhs=D,
                                 start=True, stop=True, skip_group_check=True)
                # ||a||^2 = D^2 (A1^2 x2 - 2 A1 A2 xy + A2^2 y2)
                t1 = sc("t1")
                V.tensor_mul(t1, A1, A1)
                V.tensor_mul(t1, t1, x2)
                t2 = sc("t2")
                V.tensor_mul(t2, A1, A2)
                V.tensor_mul(t2, t2, xy)
                t3 = sc("t3")
                V.tensor_mul(t3, A2, A2)
                V.tensor_mul(t3, t3, y2)
                na2 = sc("na2")
                V.scalar_tensor_tensor(out=na2, in0=t2, scalar=-2.0, in1=t1,
                                       op0=ALU.mult, op1=ALU.add)
                V.tensor_add(na2, na2, t3)
                dsq = sc("dsq")
                V.tensor_mul(dsq, D, D)
                V.tensor_mul(na2, na2, dsq)
                # q = sqrt(c * na2) with one Newton step (ACT sqrt is low precision)
                q2 = sc("q2")
                V.tensor_scalar(out=q2, in0=na2, scalar1=c_b, scalar2=None,
                                op0=ALU.mult)
                q0 = sc("q0")
                S.activation(q0, q2, AF.Sqrt)
                V.tensor_scalar_max(out=q0, in0=q0, scalar1=1e-20)
                r0 = sc("r0")
                V.reciprocal(r0, q0)
                q = sc("q")
                V.tensor_mul(q, q2, r0)
                V.tensor_add(q, q, q0)
                V.tensor_scalar_mul(out=q, in0=q, scalar1=0.5)
                arg = sc("arg")
                V.tensor_scalar_min(out=arg, in0=q, scalar1=1.0 - 1e-5)
                # artanh(arg) = 0.5 ln((1+arg)/(1-arg)); t = tanh(T*artanh)/q
                opp = sc("opp")
                V.tensor_scalar(out=opp, in0=arg, scalar1=-1.0, scalar2=1.0,
                                op0=ALU.mult, op1=ALU.add)
                opn = sc("opn")
                V.tensor_scalar_add(out=opn, in0=arg, scalar1=1.0)
                rr = sc("rr")
                V.reciprocal(rr, opp)
                rat = sc("rat")
                V.tensor_mul(rat, opn, rr)
                lg = sc("lg")
                S.activation(lg, rat, AF.Ln)
                th = sc("th")
                S.activation(th, lg, AF.Tanh, scale=T_CONST * 0.5)
                rq = sc("rq")
                V.reciprocal(rq, q)
                tm = sc("tm")
                V.tensor_mul(tm, th, rq)
                nc.tensor.matmul(stat_ps[0:1, 88:96], lhsT=onesf, rhs=tm,
                                 start=True, stop=True, skip_group_check=True)
                # <h,a> = D (A2 xy - A1 x2)
                s1_ = sc("s1_")
                V.tensor_mul(s1_, A1, x2)
                s2_ = sc("s2_")
                V.tensor_mul(s2_, A2, xy)
                ha = sc("ha")
                V.tensor_sub(ha, s2_, s1_)
                V.tensor_mul(ha, ha, D)
                hm = sc("hm")
                V.tensor_mul(hm, tm, ha)
                tsq = sc("tsq")
                V.tensor_mul(tsq, tm, tm)
                m2 = sc("m2")
                V.tensor_mul(m2, tsq, na2)
                w2s = sc("w2s")
                V.scalar_tensor_tensor(out=w2s, in0=hm, scalar=2.0, in1=m2,
                                       op0=ALU.mult, op1=ALU.add)
                B1 = sc("B1")
                V.tensor_scalar(out=B1, in0=w2s, scalar1=c_b, scalar2=1.0,
                                op0=ALU.mult, op1=ALU.add)
                p2 = sc("p2")
                V.tensor_mul(p2, x2, m2)
                den2 = sc("den2")
                V.tensor_scalar(out=den2, in0=p2, scalar1=c2_b, scalar2=1.0,
                                op0=ALU.mult, op1=ALU.add)
                V.scalar_tensor_tensor(out=den2, in0=hm, scalar=twoc_b, in1=den2,
                                       op0=ALU.mult, op1=ALU.add)
                V.tensor_scalar_add(out=den2, in0=den2, scalar1=EPS)
                D2 = sc("D2")
                V.reciprocal(D2, den2)
                g = sc("g")
                V.tensor_mul(g, A2, tm)
                V.tensor_mul(g, g, D)
                w3 = sc("w3")
                V.tensor_mul(w3, g, A1)
                V.tensor_sub(w3, B1, w3)
                # alpha -> cols 0:8, beta -> cols 8:16 of one bf16 tile; a PE
                # transpose then yields batch-linear rows for a fast DMA
                # bounce (8 contiguous 256B descriptors instead of a 2-byte
                # element scatter).
                ab_bm = sc("ab_bm", (P, 16), bf16)
                V.tensor_mul(ab_bm[:, 0:8], w3, D2)
                w4 = sc("w4")
                V.tensor_mul(w4, g, A2)
                V.tensor_mul(ab_bm[:, 8:16], w4, D2)
                nc.tensor.transpose(abT_ps, ab_bm, ident_sb)
                # warm-keeper taps pinned after the chain end: the scheduler
                # hoists dependency-free matmuls, so these read ab_bm
                for i in range(3):
                    nc.tensor.matmul(stat_ps[:, 32 + 16 * i:48 + 16 * i],
                                     lhsT=ident_sb, rhs=ab_bm,
                                     start=True, stop=True,
                                     skip_group_check=True)
                abT_sb = sc("abT_sb", (16, P), bf16)
                V.tensor_copy(abT_sb, abT_ps)
                nc.sync.dma_start(out=ab_d[0, :].rearrange("(j b) -> j b", j=8),
                                  in_=abT_sb[0:8, :])
                nc.sync.dma_start(out=ab_d[1, :].rearrange("(j b) -> j b", j=8),
                                  in_=abT_sb[8:16, :])
                for hsl in (slice(0, 512), slice(512, BL)):
                    nc.scalar.dma_start(out=alpha_b[:, hsl],
                                        in_=ab_d[0:1, hsl].to_broadcast([P, 512]))
                    nc.sync.dma_start(out=beta_b[:, hsl],
                                      in_=ab_d[1:2, hsl].to_broadcast([P, 512]))
                for i in range(2):
                    nc.tensor.matmul(dum_ps, lhsT=ident_sb,
                                     rhs=(alpha_b[:, 0:384] if i == 0
                                          else beta_b[:, 512:896]),
                                     start=(i == 0), stop=(i == 1),
                                     skip_group_check=True)
                # anchor all warm-keeper matmuls against DCE
                dum_sb = sc("dum_sb", (1, 128), bf16)
                V.tensor_copy(dum_sb, stat_ps[0:1, 24:152])
                dum_d = dram.tile([1, 128], bf16, name="dum_d")
                nc.gpsimd.dma_start(out=dum_d, in_=dum_sb)

        # psum pools (mm, stp) released here

        # ---------- z = alpha*h + beta*u (overwrites uT in place),
        # then out = z @ Wo.  Processed in two batch-column halves so the
        # MMo matmuls of half 0 overlap the DVE z-combine of half 1.
        with ExitStack() as ph2:
            mmo = ph2.enter_context(tc.tile_pool(name="mmo", bufs=8,
                                                 space="PSUM"))
            for bg in range(2):
                csl = slice(bg * 512, (bg + 1) * 512)
                with nc.named_scope(f"zcomb{bg}"):
                    for kh in range(KH):
                        t1z = zzp.tile([P, 512], bf16, name="t1z", tag="zz")
                        V.tensor_mul(t1z, hT_sb[:, kh, csl], alpha_b[:, csl])
                        t2z = zzp.tile([P, 512], bf16, name="t2z", tag="zz")
                        V.tensor_mul(t2z, uT_sb[:, kh, csl], beta_b[:, csl])
                        V.tensor_add(uT_sb[:, kh, csl], t1z, t2z)
                with nc.named_scope(f"mmo{bg}"):
                    pso = [mmo.tile([P, 500], f32, name=f"pso{bg}_{i}",
                                    tag="mmo") for i in range(8)]
                    for kh in range(KH):
                        wot = wop.tile([P, OUT], bf16, name="wot", tag="wo")
                        nc.sync.dma_start(out=wot, in_=wo_d[kh])
                        for i in range(4):
                            b = bg * 4 + i
                            for och in range(2):
                                nc.tensor.matmul(
                                    pso[i * 2 + och],
                                    lhsT=uT_sb[:, kh, b * P:(b + 1) * P],
                                    rhs=wot[:, och * 500:(och + 1) * 500],
                                    start=(kh == 0), stop=(kh == KH - 1))
                    for i in range(4):
                        b = bg * 4 + i
                        for och in range(2):
                            # stage drains across the scr ring and the (now
                            # dead) alpha/beta tiles for 4-deep DMA pipelining
                            slot = (i * 2 + och) % 3
                            if slot == 0:
                                ob = scr.tile([P, 500], f32, name="ob",
                                              tag="hh")
                            elif slot == 1:
                                ob = abp.tile([P, 500], f32, name="ob",
                                              tag="alpha_b")
                            else:
                                ob = abp.tile([P, 500], f32, name="ob",
                                              tag="beta_b")
                            if och == 0:
                                S.copy(ob, pso[i * 2])
                            else:
                                V.tensor_copy(ob, pso[i * 2 + 1])
                            outq = nc.sync if och == 0 else nc.scalar
                            outq.dma_start(
                                out=out_d[b * P:(b + 1) * P,
                                          och * 500:(och + 1) * 500],
                                in_=ob)

    nc.compile()
    return nc


def _get_nc(with_b1, with_b2):
    for k, v in _nc_cache:
        if k == (with_b1, with_b2):
            return v
    nc = _build(with_b1, with_b2)
    _nc_cache.append(((with_b1, with_b2), nc))
    return nc


def kernel(x, W1, b1, W2, b2, Wo, bo, cp_w1, cp_b1, cp_w2, cp_b2,
           _trace=False, _tmpdir=None):
    x = np.asarray(x, dtype=np.float32)
    with_b1 = bool(np.any(b1))
    with_b2 = bool(np.any(b2))
    nc = _get_nc(with_b1, with_b2)

    # W1 pre-scaled x1024 so fp8 and bf16 parts share one accumulator
    w1s = np.asarray(W1, np.float32) * W2S
    # fp8 rows 0:1024 in DoubleRow pair layout [KH, P, KQ//2, 2, P]
    w1q_t = np.ascontiguousarray(
        np.clip(w1s[:KQ * P], -E4MAX, E4MAX)
        .reshape(KQ, P, KH, P).transpose(2, 1, 0, 3)
    ).astype(E4).reshape(KH, P, KQ // 2, 2, P)
    # bf16 rows 1024:3072 in halves [KH, 2, P, KB//2, P]
    w1_t = np.ascontiguousarray(
        w1s[KQ * P:].reshape(KB, P, KH, P).transpose(2, 1, 0, 3)
        .reshape(KH, P, 2, KB // 2, P).transpose(0, 2, 1, 3, 4)
    ).astype(BF)
    # w2r[mh, p, kh, q] = W2[kh*128+p, mh*128+q], fp8 e4m3 scaled x1024
    w2_t = np.ascontiguousarray(
        np.clip(np.asarray(W2, np.float32) * W2S, -E4MAX, E4MAX)
        .reshape(KH, P, KH, P).transpose(2, 1, 0, 3)
    ).astype(E4)
    wo_t = np.asarray(Wo, np.float32).reshape(KH, P, OUT).astype(BF)
    cpw1_t = np.ascontiguousarray(np.clip(
        np.asarray(cp_w1, np.float32).T * 64.0, -E4MAX, E4MAX)
        .reshape(KI, P, 16)).astype(E4)
    cpw2_t = np.asarray(cp_w2, np.float32).reshape(1, 16).T.astype(BF)
    cpw2_t = np.ascontiguousarray(cpw2_t)
    cpb1_t = np.asarray(cp_b1, np.float32).reshape(16, 1)
    cpb2_t = np.asarray(cp_b2, np.float32).reshape(1, 1)
    b1_t = np.ascontiguousarray(np.asarray(b1, np.float32).reshape(KH, P).T)
    b2_t = np.ascontiguousarray(np.asarray(b2, np.float32).reshape(KH, P).T)

    in_maps = []
    for c in range(N_CORES):
        shard = x[c * BL:(c + 1) * BL]
        shT = np.ascontiguousarray(shard.T)
        xq_c = np.clip(shT[:KQ * P], -E4MAX, E4MAX).reshape(KQ, P, BL).astype(E4)
        xT = shT[KQ * P:].reshape(KB, P, BL).astype(BF)
        m = {"xq": xq_c, "xT": xT, "w1": w1_t, "w1q": w1q_t, "w2": w2_t,
             "wo": wo_t,
             "cpw1": cpw1_t, "cpw2": cpw2_t, "cpb1": cpb1_t, "cpb2": cpb2_t,
             "ident": _IDENT}
        if with_b1:
            m["b1"] = b1_t
        if with_b2:
            m["b2"] = b2_t
        in_maps.append(m)

    kw = {}
    if _trace:
        kw = dict(trace=True, tmpdir=_tmpdir or tempfile.mkdtemp(prefix="cdk_"))
    res = run_bass_kernel_spmd(nc, in_maps, list(range(N_CORES)), **kw)

    out = np.concatenate([res.results[c]["out"] for c in range(N_CORES)], axis=0)
    bo = np.asarray(bo, np.float32)
    if np.any(bo):
        out = out + bo
    if _trace:
        kernel._last_result = res
    return out


# revision 24
# speedup vs baseline: 1.5101x; 1.0553x over previous
"""Trainium2 Bass kernel for nn_ConservativeDynamicCurvatureMLP.

Data-parallel over 8 NeuronCores: the batch (8192) is sharded into 8
local shards of 1024 rows; all weights are replicated. The curvature
scalar c_avg couples the shards through a global mean, handled with a
single-scalar AllReduce.

Math (reference):
    h = tanh(x @ W1 + b1)
    u = sigmoid(h @ W2 + b2)
    c = clip(mean(MIN_C + (MAX_C-MIN_C) * sigmoid(relu(x@cp_w1.T+cp_b1)@cp_w2.T+cp_b2)), MIN_C, MAX_C)
    z = poincare_ball_layer(h, u, c, T)
    out = z @ Wo + bo

The poincare layer collapses algebraically to z = alpha(row)*h + beta(row)*u
where alpha/beta are scalar functions of the row statistics
x2=||h||^2, y2=||u||^2, xy=<h,u> and c.  The NaN fallback is measure-zero
and omitted.

Perf notes (measured on hw):
  - With 8 cores active the PE is power-throttled to ~2.0 GHz (HAM state
    31), so bf16 N=512 matmuls pace at ~263 ns.  The only lever is fewer
    PE cycles: MM2 (h@W2, the largest matmul) runs as fp8-e4m3 DoubleRow
    (256-deep contraction per pass, 2x FLOP rate, measured 216 ns/MM at
    2.4 GHz in isolation).  W2 is host-converted to e4m3 * 1024 (clipped
    to +-240, the TRN e4m3 max); the 1/1024 folds into the sigmoid's
    activation scale.  h is cast bf16->fp8 per row-tile on the DVE.
    MM1 / MMo stay bf16: fp8 there pushes rel-err too close to the 2e-2
    gate (simulated 1.96e-2 vs 1.26e-2 for MM2-only).
  - Row statistics go batch-major directly via tiny stationary-squares
    matmuls (lhsT = accumulated squares block, rhs = ones) instead of a
    feature-major ones-reduction + DRAM bounce, shortening the serial
    stats -> alpha/beta tail.  The per-row scalar chain runs once on
    [128, 8] tiles reading stats straight from SBUF.
"""

import tempfile
from contextlib import ExitStack

import numpy as np
import ml_dtypes

import concourse.bass as bass
import concourse.bacc as bacc
import concourse.mybir as mybir
import concourse.tile as tile
from concourse.bass_utils import run_bass_kernel_spmd

P = 128
N_CORES = 8
B_FULL = 8192
BL = B_FULL // N_CORES          # 1024 rows per core
IN = 3072
HID = 4096
OUT = 1000
KI = IN // P                    # 24
KQ = 12                         # leading ki-groups of MM1 done in fp8
KB = KI - KQ                    # trailing ki-groups in bf16
KH = HID // P                   # 32
NB = BL // P                    # 8 batch tiles
MIN_C = 0.001 * 0.5
MAX_C = 0.001 * 2.0
T_CONST = 0.7
EPS = 1e-7
W2S = 1024.0                    # host-side fp8 scale for W2
E4MAX = 240.0                   # TRN e4m3 saturation

dt = mybir.dt
AF = mybir.ActivationFunctionType
ALU = mybir.AluOpType
PM = mybir.MatmulPerfMode
BF = ml_dtypes.bfloat16
E4 = ml_dtypes.float8_e4m3

_nc_cache = []
_IDENT = np.eye(P, dtype=ml_dtypes.bfloat16)


def _build(with_b1, with_b2):
    nc = bacc.Bacc("TRN2", target_bir_lowering=False, debug=False,
                   num_devices=N_CORES)

    # x features 0:1024 as fp8 (partial-fp8 MM1), 1024:3072 as bf16
    xq_d = nc.dram_tensor("xq", [KQ, P, BL], dt.float8e4, kind="ExternalInput")
    xT_d = nc.dram_tensor("xT", [KB, P, BL], dt.bfloat16, kind="ExternalInput")
    # W1 scaled x1024 throughout (so fp8 and bf16 parts share one psum
    # accumulator); fp8 rows in DoubleRow pair layout, bf16 rows in halves
    w1q_d = nc.dram_tensor("w1q", [KH, P, KQ // 2, 2, P], dt.float8e4,
                           kind="ExternalInput")
    w1_d = nc.dram_tensor("w1", [KH, P, KB, P], dt.bfloat16,
                          kind="ExternalInput")
    # W2 in fp8 (x1024): w2r[mh, p, kh, q] = W2[kh*128+p, mh*128+q]
    w2_d = nc.dram_tensor("w2", [KH, P, KH, P], dt.float8e4, kind="ExternalInput")
    wo_d = nc.dram_tensor("wo", [KH, P, OUT], dt.bfloat16, kind="ExternalInput")
    cpw1_d = nc.dram_tensor("cpw1", [KI, P, 16], dt.float8e4, kind="ExternalInput")
    cpw2_d = nc.dram_tensor("cpw2", [16, 1], dt.bfloat16, kind="ExternalInput")
    cpb1_d = nc.dram_tensor("cpb1", [16, 1], dt.float32, kind="ExternalInput")
    cpb2_d = nc.dram_tensor("cpb2", [1, 1], dt.float32, kind="ExternalInput")
    ident_d = nc.dram_tensor("ident", [P, P], dt.bfloat16, kind="ExternalInput")
    b1_d = nc.dram_tensor("b1", [P, KH], dt.float32, kind="ExternalInput") if with_b1 else None
    b2_d = nc.dram_tensor("b2", [P, KH], dt.float32, kind="ExternalInput") if with_b2 else None
    out_d = nc.dram_tensor("out", [BL, OUT], dt.float32, kind="ExternalOutput")

    f32 = dt.float32
    bf16 = dt.bfloat16
    fp8 = dt.float8e4

    with tile.TileContext(nc) as tc, ExitStack() as ctx:
        const = ctx.enter_context(tc.tile_pool(name="const", bufs=1))
        big = ctx.enter_context(tc.tile_pool(name="big", bufs=1))
        htp = ctx.enter_context(tc.tile_pool(name="htp", bufs=1))
        hqp = ctx.enter_context(tc.tile_pool(name="hqp", bufs=1))
        wp = ctx.enter_context(tc.tile_pool(name="wp", bufs=2))
        wop = ctx.enter_context(tc.tile_pool(name="wop", bufs=4))
        scr = ctx.enter_context(tc.tile_pool(name="scr", bufs=2))
        zzp = ctx.enter_context(tc.tile_pool(name="zzp", bufs=3))
        sacc = ctx.enter_context(tc.tile_pool(name="sacc", bufs=1))
        abp = ctx.enter_context(tc.tile_pool(name="abp", bufs=1))
        scal = ctx.enter_context(tc.tile_pool(name="scal", bufs=1))
        cpp = ctx.enter_context(tc.tile_pool(name="cpp", bufs=1))
        dram = ctx.enter_context(tc.tile_pool(name="dram", bufs=1, space="DRAM"))

        V = nc.vector
        S = nc.scalar

        def sc(name, shape=(P, 8), dtype=f32):
            return scal.tile(list(shape), dtype, name=name, tag=name)

        # ---------- persistent activations (feature-major) ----------
        onesb = const.tile([P, 1], bf16, name="onesb")
        nc.vector.memset(onesb, 1.0)
        onesf = const.tile([P, 1], f32, name="onesf")
        nc.vector.memset(onesf, 1.0)
        # one 64KB slot holds bf16 x (ki 0:KB), fp8 x (as bitcast bytes in
        # the tail), and is later recycled as uT
        xall = big.tile([P, KH, BL], bf16, name="xall", tag="big")
        xT_sb = xall[:, 0:KB, :]
        # first weight half-row issues on the sync queue before anything else;
        # xT streams on the gpsimd + scalar queues in parallel (the load is
        # HBM-bandwidth-bound, ~35us for 6.3MB with all 8 cores pulling)
        w1q0 = wp.tile([P, KQ // 2, 2, P], fp8, name="w1qrow", tag="w1q")
        nc.sync.dma_start(out=w1q0, in_=w1q_d[0])
        xq_sb = (xall[:, KB:KB + KQ // 2, :].bitcast(fp8)
                 .rearrange("p a (t b) -> p (a t) b", t=2))
        queues = [nc.scalar, nc.gpsimd]
        for idx, (a, b) in enumerate(((0, 3), (3, 6), (6, 9), (9, 12))):
            queues[idx % 2].dma_start(
                out=xq_sb[:, a:b, :],
                in_=xq_d[a:b].rearrange("k p b -> p k b"))
        for idx, (a, b) in enumerate(((0, 2), (2, 4), (4, 6), (6, 8),
                                      (8, 10), (10, 12))):
            queues[idx % 2].dma_start(
                out=xT_sb[:, a:b, :],
                in_=xT_d[a:b].rearrange("k p b -> p k b"))
        ident_sb = const.tile([P, P], bf16, name="ident_sb")
        nc.sync.dma_start(out=ident_sb, in_=ident_d[:, :])
        hT_sb = htp.tile([P, KH, BL], bf16, name="hT_sb")
        hTq_sb = hqp.tile([P, KH, BL], fp8, name="hTq_sb")
        if with_b1:
            b1_sb = const.tile([P, KH], f32, name="b1_sb")
            nc.sync.dma_start(out=b1_sb, in_=b1_d[:, :])
        if with_b2:
            b2_sb = const.tile([P, KH], f32, name="b2_sb")
            nc.sync.dma_start(out=b2_sb, in_=b2_d[:, :])

        with ExitStack() as ph1:
            mm = ph1.enter_context(tc.tile_pool(name="mm", bufs=3, space="PSUM"))
            stp = ph1.enter_context(tc.tile_pool(name="stp", bufs=1, space="PSUM"))
            # one bank shared by the batch-major stats (cols 0:8 x2,
            # 8:16 y2, 16:24 xy) and the HAM warm-keeper matmul target
            # (cols 128:512); plus the a/b transpose target bank
            stat_ps = stp.tile([P, 512], f32, name="stat_ps")
            dum_ps = stat_ps[:, 128:512]
            abT_ps = stp.tile([16, P], bf16, name="abT_ps")

            # ---------- MM1: hT = tanh(W1.T @ xT) (bf16) ----------
            x2a = sacc.tile([P, BL], bf16, name="x2a")
            y2a = sacc.tile([P, BL], bf16, name="y2a")
            xya = sacc.tile([P, BL], bf16, name="xya")
            with nc.named_scope("mm1"):
                for mh in range(KH):
                    ps = mm.tile([P, BL], f32, name="ps", tag="mm")
                    if mh == 0:
                        w1q = w1q0
                    else:
                        w1q = wp.tile([P, KQ // 2, 2, P], fp8, name="w1qrow",
                                      tag="w1q")
                        nc.sync.dma_start(out=w1q, in_=w1q_d[mh])
                    w1r = wp.tile([P, KB, P], bf16, name="w1row", tag="w1")
                    nc.sync.dma_start(out=w1r, in_=w1_d[mh])
                    for hsl in (slice(0, 512), slice(512, BL)):
                        for j in range(KQ // 2):
                            nc.tensor.matmul(
                                ps[:, hsl], lhsT=w1q[:, j, :, :],
                                rhs=xq_sb[:, 2 * j:2 * j + 2, hsl],
                                start=(j == 0), stop=False,
                                perf_mode=PM.DoubleRow,
                                skip_group_check=True)
                        for ki in range(KB):
                            nc.tensor.matmul(ps[:, hsl], lhsT=w1r[:, ki, :],
                                             rhs=xT_sb[:, ki, hsl],
                                             start=False, stop=(ki == KB - 1),
                                             skip_group_check=True)
                    if with_b1:
                        S.activation(hT_sb[:, mh, :], ps, AF.Tanh,
                                     bias=b1_sb[:, mh:mh + 1], scale=1.0 / W2S)
                    else:
                        S.activation(hT_sb[:, mh, :], ps, AF.Tanh,
                                     scale=1.0 / W2S)
                    # fp8 copy for MM2's DoubleRow rhs
                    V.tensor_copy(hTq_sb[:, mh, :], hT_sb[:, mh, :])
                    hh = scr.tile([P, BL], bf16, name="hh", tag="hh")
                    S.activation(hh, hT_sb[:, mh, :], AF.Square)
                    if mh == 0:
                        V.tensor_copy(x2a, hh)
                    else:
                        V.tensor_add(x2a, x2a, hh)
                # x2 -> batch-major: out[b,0] = sum_p x2a[p, blk*128+b]
                for blk in range(NB):
                    nc.tensor.matmul(stat_ps[:, blk:blk + 1],
                                     lhsT=x2a[:, blk * P:(blk + 1) * P],
                                     rhs=onesb, start=True, stop=True,
                                     skip_group_check=True)

            # ---------- curvature predictor (xT still resident; the
            # AllReduce hides under MM1/MM2) ----------
            with nc.named_scope("cp"):
                cpw1_sb = const.tile([P, KI, 16], fp8, name="cpw1_sb")
                nc.sync.dma_start(out=cpw1_sb,
                                  in_=cpw1_d.rearrange("k p q -> p k q"))
                cpw2_sb = const.tile([16, 1], bf16, name="cpw2_sb")
                nc.sync.dma_start(out=cpw2_sb, in_=cpw2_d[:, :])
                cpb1_sb = const.tile([16, 1], f32, name="cpb1_sb")
                nc.sync.dma_start(out=cpb1_sb, in_=cpb1_d[:, :])
                cpb2_sb = const.tile([1, 1], f32, name="cpb2_sb")
                nc.sync.dma_start(out=cpb2_sb, in_=cpb2_d[:, :])
                cph_sb = cpp.tile([16, BL], fp8, name="cph_sb")
                for ch in range(2):
                    cps = mm.tile([16, 512], f32, name="cps", tag="mm")
                    for ki in range(KI):
                        xr = (xq_sb[:, ki, ch * 512:(ch + 1) * 512] if ki < KQ
                              else xT_sb[:, ki - KQ, ch * 512:(ch + 1) * 512])
                        nc.tensor.matmul(
                            cps, lhsT=cpw1_sb[:, ki, :], rhs=xr,
                            start=(ki == 0), stop=(ki == KI - 1))
                    S.activation(cph_sb[:, ch * 512:(ch + 1) * 512], cps,
                                 AF.Relu, bias=cpb1_sb, scale=1.0 / 64.0)
                sparts = []
                for ch in range(2):
                    c2p = mm.tile([1, 512], f32, name="c2p", tag="mm")
                    nc.tensor.matmul(c2p, lhsT=cpw2_sb,
                                     rhs=cph_sb[:16, ch * 512:(ch + 1) * 512],
                                     start=True, stop=True)
                    cpw = cpp.tile([1, 512], bf16, name="cpw", tag="cpw")
                    spart = cpp.tile([1, 1], f32, name=f"spart{ch}",
                                     tag=f"spart{ch}")
                    S.activation(cpw, c2p, AF.Sigmoid, bias=cpb2_sb,
                                 accum_out=spart)
                    sparts.append(spart)
                s_loc = cpp.tile([1, 1], f32, name="s_loc")
                V.tensor_add(s_loc, sparts[0], sparts[1])
                cin = dram.tile([1, 1], f32, name="cin")
                cout = dram.tile([1, 1], f32, name="cout")
                nc.sync.dma_start(out=cin, in_=s_loc)
                nc.gpsimd.collective_compute(
                    "AllReduce", ALU.add,
                    replica_groups=[list(range(N_CORES))],
                    ins=[cin.opt()], outs=[cout.opt()])
                s_b = sc("s_b", (P, 1))
                nc.gpsimd.dma_start(out=s_b, in_=cout.to_broadcast([P, 1]))
                # c = clip(MIN_C + (MAX_C-MIN_C)*mean(c_pred))
                c_b = sc("c_b", (P, 1))
                V.tensor_scalar(out=c_b, in0=s_b,
                                scalar1=(MAX_C - MIN_C) / B_FULL,
                                scalar2=MIN_C, op0=ALU.mult, op1=ALU.add)
                V.tensor_scalar_min(out=c_b, in0=c_b, scalar1=MAX_C)
                V.tensor_scalar_max(out=c_b, in0=c_b, scalar1=MIN_C)
                negc_b = sc("negc_b", (P, 1))
                V.tensor_scalar_mul(out=negc_b, in0=c_b, scalar1=-1.0)
                twoc_b = sc("twoc_b", (P, 1))
                V.tensor_scalar_mul(out=twoc_b, in0=c_b, scalar1=2.0)
                neg2c_b = sc("neg2c_b", (P, 1))
                V.tensor_scalar_mul(out=neg2c_b, in0=c_b, scalar1=-2.0)
                c2_b = sc("c2_b", (P, 1))
                V.tensor_mul(c2_b, c_b, c_b)

            # ---------- MM2: uT = sigmoid(W2.T @ hT) fp8 DoubleRow ----------
            uT_sb = big.tile([P, KH, BL], bf16, name="uT_sb", tag="big")
            inv_s2 = 1.0 / W2S
            with nc.named_scope("mm2"):
                for mh in range(KH):
                    ps = mm.tile([P, BL], f32, name="ps", tag="mm")
                    w2row = wp.tile([P, KH, P], fp8, name="w2row", tag="w2")
                    nc.sync.dma_start(out=w2row, in_=w2_d[mh])
                    for j in range(KH // 2):
                        nc.tensor.matmul(ps[:, 0:512],
                                         lhsT=w2row[:, 2 * j:2 * j + 2, :],
                                         rhs=hTq_sb[:, 2 * j:2 * j + 2, 0:512],
                                         start=(j == 0), stop=(j == KH // 2 - 1),
                                         perf_mode=PM.DoubleRow)
                        nc.tensor.matmul(ps[:, 512:BL],
                                         lhsT=w2row[:, 2 * j:2 * j + 2, :],
                                         rhs=hTq_sb[:, 2 * j:2 * j + 2, 512:BL],
                                         start=(j == 0), stop=(j == KH // 2 - 1),
                                         perf_mode=PM.DoubleRow)
                    if with_b2:
                        S.activation(uT_sb[:, mh, :], ps, AF.Sigmoid,
                                     bias=b2_sb[:, mh:mh + 1], scale=inv_s2)
                    else:
                        S.activation(uT_sb[:, mh, :], ps, AF.Sigmoid,
                                     scale=inv_s2)
                    uu = scr.tile([P, BL], bf16, name="uu", tag="hh")
                    S.activation(uu, uT_sb[:, mh, :], AF.Square)
                    hu = scr.tile([P, BL], bf16, name="hu", tag="hh")
                    V.tensor_mul(hu, hT_sb[:, mh, :], uT_sb[:, mh, :])
                    if mh == 0:
                        V.tensor_copy(y2a, uu)
                        V.tensor_copy(xya, hu)
                    else:
                        V.tensor_add(y2a, y2a, uu)
                        V.tensor_add(xya, xya, hu)
                # warm-keeper group 1: bridges the PE from mm2's last matmul
                # across the stats-accumulation drain (a >3.4us idle would
                # HAM-throttle the clock for the MMo matmuls).  A real
                # accumulation group so DCE cannot drop it.
                for i in range(14):
                    nc.tensor.matmul(dum_ps, lhsT=ident_sb,
                                     rhs=hT_sb[:, i, 0:384],
                                     start=(i == 0), stop=(i == 13),
                                     skip_group_check=True)
                for blk in range(NB):
                    nc.tensor.matmul(stat_ps[:, 8 + blk:9 + blk],
                                     lhsT=y2a[:, blk * P:(blk + 1) * P],
                                     rhs=onesb, start=True, stop=True,
                                     skip_group_check=True)
                    nc.tensor.matmul(stat_ps[:, 16 + blk:17 + blk],
                                     lhsT=xya[:, blk * P:(blk + 1) * P],
                                     rhs=onesb, start=True, stop=True,
                                     skip_group_check=True)

            # ---------- per-row scalar chain, batch-major [128, 8] ----------
            alpha_b = abp.tile([P, BL], bf16, name="alpha_b", tag="alpha_b")
            beta_b = abp.tile([P, BL], bf16, name="beta_b", tag="beta_b")
            ab_d = dram.tile([2, BL], bf16, name="ab_d")

            with nc.named_scope("scalars"):
                stats_sb = sc("stats_sb", (P, 24))
                V.tensor_copy(stats_sb, stat_ps[:, 0:24])
                x2 = stats_sb[:, 0:8]
                y2 = stats_sb[:, 8:16]
                xy = stats_sb[:, 16:24]
                w = sc("w")
                V.scalar_tensor_tensor(out=w, in0=xy, scalar=-2.0, in1=y2,
                                       op0=ALU.mult, op1=ALU.add)
                A1 = sc("A1")
                V.tensor_scalar(out=A1, in0=w, scalar1=c_b, scalar2=1.0,
                                op0=ALU.mult, op1=ALU.add)
                A2 = sc("A2")
                V.tensor_scalar(out=A2, in0=x2, scalar1=negc_b, scalar2=1.0,
                                op0=ALU.mult, op1=ALU.add)
                p1 = sc("p1")
                V.tensor_mul(p1, x2, y2)
                den = sc("den")
                V.tensor_scalar(out=den, in0=p1, scalar1=c2_b, scalar2=1.0,
                                op0=ALU.mult, op1=ALU.add)
                V.scalar_tensor_tensor(out=den, in0=xy, scalar=neg2c_b, in1=den,
                                       op0=ALU.mult, op1=ALU.add)
                V.tensor_scalar_add(out=den, in0=den, scalar1=EPS)
                D = sc("D")
                V.reciprocal(D, den)
                nc.tensor.matmul(stat_ps[0:1, 80:88], lhsT=onesf, rhs=D,
                                 start=True, stop=True, skip_group_check=True)
                # ||a||^2 = D^2 (A1^2 x2 - 2 A1 A2 xy + A2^2 y2)
                t1 = sc("t1")
                V.tensor_mul(t1, A1, A1)
                V.tensor_mul(t1, t1, x2)
                t2 = sc("t2")
                V.tensor_mul(t2, A1, A2)
                V.tensor_mul(t2, t2, xy)
                t3 = sc("t3")
                V.tensor_mul(t3, A2, A2)
                V.tensor_mul(t3, t3, y2)
                na2 = sc("na2")
                V.scalar_tensor_tensor(out=na2, in0=t2, scalar=-2.0, in1=t1,
                                       op0=ALU.mult, op1=ALU.add)
                V.tensor_add(na2, na2, t3)
                dsq = sc("dsq")
                V.tensor_mul(dsq, D, D)
                V.tensor_mul(na2, na2, dsq)
                # q = sqrt(c * na2) with one Newton step (ACT sqrt is low precision)
                q2 = sc("q2")
                V.tensor_scalar(out=q2, in0=na2, scalar1=c_b, scalar2=None,
                                op0=ALU.mult)
                q0 = sc("q0")
                S.activation(q0, q2, AF.Sqrt)
                V.tensor_scalar_max(out=q0, in0=q0, scalar1=1e-20)
                r0 = sc("r0")
                V.reciprocal(r0, q0)
                q = sc("q")
                V.tensor_mul(q, q2, r0)
                V.tensor_add(q, q, q0)
                V.tensor_scalar_mul(out=q, in0=q, scalar1=0.5)
                arg = sc("arg")
                V.tensor_scalar_min(out=arg, in0=q, scalar1=1.0 - 1e-5)
                # artanh(arg) = 0.5 ln((1+arg)/(1-arg)); t = tanh(T*artanh)/q
                opp = sc("opp")
                V.tensor_scalar(out=opp, in0=arg, scalar1=-1.0, scalar2=1.0,
                                op0=ALU.mult, op1=ALU.add)
                opn = sc("opn")
                V.tensor_scalar_add(out=opn, in0=arg, scalar1=1.0)
                rr = sc("rr")
                V.reciprocal(rr, opp)
                rat = sc("rat")
                V.tensor_mul(rat, opn, rr)
                lg = sc("lg")
                S.activation(lg, rat, AF.Ln)
                th = sc("th")
                S.activation(th, lg, AF.Tanh, scale=T_CONST * 0.5)
                rq = sc("rq")
                V.reciprocal(rq, q)
                tm = sc("tm")
                V.tensor_mul(tm, th, rq)
                nc.tensor.matmul(stat_ps[0:1, 88:96], lhsT=onesf, rhs=tm,
                                 start=True, stop=True, skip_group_check=True)
                # <h,a> = D (A2 xy - A1 x2)
                s1_ = sc("s1_")
                V.tensor_mul(s1_, A1, x2)
                s2_ = sc("s2_")
                V.tensor_mul(s2_, A2, xy)
                ha = sc("ha")
                V.tensor_sub(ha, s2_, s1_)
                V.tensor_mul(ha, ha, D)
                hm = sc("hm")
                V.tensor_mul(hm, tm, ha)
                tsq = sc("tsq")
                V.tensor_mul(tsq, tm, tm)
                m2 = sc("m2")
                V.tensor_mul(m2, tsq, na2)
                w2s = sc("w2s")
                V.scalar_tensor_tensor(out=w2s, in0=hm, scalar=2.0, in1=m2,
                                       op0=ALU.mult, op1=ALU.add)
                B1 = sc("B1")
                V.tensor_scalar(out=B1, in0=w2s, scalar1=c_b, scalar2=1.0,
                                op0=ALU.mult, op1=ALU.add)
                p2 = sc("p2")
                V.tensor_mul(p2, x2, m2)
                den2 = sc("den2")
                V.tensor_scalar(out=den2, in0=p2, scalar1=c2_b, scalar2=1.0,
                                op0=ALU.mult, op1=ALU.add)
                V.scalar_tensor_tensor(out=den2, in0=hm, scalar=twoc_b, in1=den2,
                                       op0=ALU.mult, op1=ALU.add)
                V.tensor_scalar_add(out=den2, in0=den2, scalar1=EPS)
                D2 = sc("D2")
                V.reciprocal(D2, den2)
                g = sc("g")
                V.tensor_mul(g, A2, tm)
                V.tensor_mul(g, g, D)
                w3 = sc("w3")
                V.tensor_mul(w3, g, A1)
                V.tensor_sub(w3, B1, w3)
                # alpha -> cols 0:8, beta -> cols 8:16 of one bf16 tile; a PE
                # transpose then yields batch-linear rows for a fast DMA
                # bounce (8 contiguous 256B descriptors instead of a 2-byte
                # element scatter).
                ab_bm = sc("ab_bm", (P, 16), bf16)
                V.tensor_mul(ab_bm[:, 0:8], w3, D2)
                w4 = sc("w4")
                V.tensor_mul(w4, g, A2)
                V.tensor_mul(ab_bm[:, 8:16], w4, D2)
                nc.tensor.transpose(abT_ps, ab_bm, ident_sb)
                # warm-keeper taps pinned after the chain end: the scheduler
                # hoists dependency-free matmuls, so these read ab_bm
                for i in range(3):
                    nc.tensor.matmul(stat_ps[:, 32 + 16 * i:48 + 16 * i],
                                     lhsT=ident_sb, rhs=ab_bm,
                                     start=True, stop=True,
                                     skip_group_check=True)
                abT_sb = sc("abT_sb", (16, P), bf16)
                V.tensor_copy(abT_sb, abT_ps)
                nc.sync.dma_start(out=ab_d[0, :].rearrange("(j b) -> j b", j=8),
                                  in_=abT_sb[0:8, :])
                nc.sync.dma_start(out=ab_d[1, :].rearrange("(j b) -> j b", j=8),
                                  in_=abT_sb[8:16, :])
                for hsl in (slice(0, 512), slice(512, BL)):
                    nc.scalar.dma_start(out=alpha_b[:, hsl],
                                        in_=ab_d[0:1, hsl].to_broadcast([P, 512]))
                    nc.sync.dma_start(out=beta_b[:, hsl],
                                      in_=ab_d[1:2, hsl].to_broadcast([P, 512]))
                for i in range(2):
                    nc.tensor.matmul(dum_ps, lhsT=ident_sb,
                                     rhs=(alpha_b[:, 0:384] if i == 0
                                          else beta_b[:, 512:896]),
                                     start=(i == 0), stop=(i == 1),
                                     skip_group_check=True)
                # anchor all warm-keeper matmuls against DCE
                dum_sb = sc("dum_sb", (1, 128), bf16)
                V.tensor_copy(dum_sb, stat_ps[0:1, 24:152])
                dum_d = dram.tile([1, 128], bf16, name="dum_d")
                nc.gpsimd.dma_start(out=dum_d, in_=dum_sb)

        # psum pools (mm, stp) released here

        # ---------- z = alpha*h + beta*u (overwrites uT in place),
        # then out = z @ Wo.  Processed in two batch-column halves so the
        # MMo matmuls of half 0 overlap the DVE z-combine of half 1.
        with ExitStack() as ph2:
            mmo = ph2.enter_context(tc.tile_pool(name="mmo", bufs=8,
                                                 space="PSUM"))
            for bg in range(2):
                csl = slice(bg * 512, (bg + 1) * 512)
                with nc.named_scope(f"zcomb{bg}"):
                    for kh in range(KH):
                        t1z = zzp.tile([P, 512], bf16, name="t1z", tag="zz")
                        V.tensor_mul(t1z, hT_sb[:, kh, csl], alpha_b[:, csl])
                        t2z = zzp.tile([P, 512], bf16, name="t2z", tag="zz")
                        V.tensor_mul(t2z, uT_sb[:, kh, csl], beta_b[:, csl])
                        V.tensor_add(uT_sb[:, kh, csl], t1z, t2z)
                with nc.named_scope(f"mmo{bg}"):
                    pso = [mmo.tile([P, 500], f32, name=f"pso{bg}_{i}",
                                    tag="mmo") for i in range(8)]
                    for kh in range(KH):
                        wot = wop.tile([P, OUT], bf16, name="wot", tag="wo")
                        nc.sync.dma_start(out=wot, in_=wo_d[kh])
                        for i in range(4):
                            b = bg * 4 + i
                            for och in range(2):
                                nc.tensor.matmul(
                                    pso[i * 2 + och],
                                    lhsT=uT_sb[:, kh, b * P:(b + 1) * P],
                                    rhs=wot[:, och * 500:(och + 1) * 500],
                                    start=(kh == 0), stop=(kh == KH - 1))
                    for i in range(4):
                        b = bg * 4 + i
                        for och in range(2):
                            # stage drains across the scr ring and the (now
                            # dead) alpha/beta tiles for 4-deep DMA pipelining
                            slot = (i * 2 + och) % 3
                            if slot == 0:
                                ob = scr.tile([P, 500], f32, name="ob",
                                              tag="hh")
                            elif slot == 1:
                                ob = abp.tile([P, 500], f32, name="ob",
                                              tag="alpha_b")
                            else:
                                ob = abp.tile([P, 500], f32, name="ob",
                                              tag="beta_b")
                            if och == 0:
                                S.copy(ob, pso[i * 2])
                            else:
                                V.tensor_copy(ob, pso[i * 2 + 1])
                            outq = nc.sync if och == 0 else nc.scalar
                            outq.dma_start(
                                out=out_d[b * P:(b + 1) * P,
                                          och * 500:(och + 1) * 500],
                                in_=ob)

    nc.compile()
    return nc


def _get_nc(with_b1, with_b2):
    for k, v in _nc_cache:
        if k == (with_b1, with_b2):
            return v
    nc = _build(with_b1, with_b2)
    _nc_cache.append(((with_b1, with_b2), nc))
    return nc


def kernel(x, W1, b1, W2, b2, Wo, bo, cp_w1, cp_b1, cp_w2, cp_b2,
           _trace=False, _tmpdir=None):
    x = np.asarray(x, dtype=np.float32)
    with_b1 = bool(np.any(b1))
    with_b2 = bool(np.any(b2))
    nc = _get_nc(with_b1, with_b2)

    # W1 pre-scaled x1024 so fp8 and bf16 parts share one accumulator
    w1s = np.asarray(W1, np.float32) * W2S
    # fp8 rows 0:1024 in DoubleRow pair layout [KH, P, KQ//2, 2, P]
    w1q_t = np.ascontiguousarray(
        np.clip(w1s[:KQ * P], -E4MAX, E4MAX)
        .reshape(KQ, P, KH, P).transpose(2, 1, 0, 3)
    ).astype(E4).reshape(KH, P, KQ // 2, 2, P)
    # bf16 rows [KH, P, KB, P]
    w1_t = np.ascontiguousarray(
        w1s[KQ * P:].reshape(KB, P, KH, P).transpose(2, 1, 0, 3)
    ).astype(BF)
    # w2r[mh, p, kh, q] = W2[kh*128+p, mh*128+q], fp8 e4m3 scaled x1024
    w2_t = np.ascontiguousarray(
        np.clip(np.asarray(W2, np.float32) * W2S, -E4MAX, E4MAX)
        .reshape(KH, P, KH, P).transpose(2, 1, 0, 3)
    ).astype(E4)
    wo_t = np.asarray(Wo, np.float32).reshape(KH, P, OUT).astype(BF)
    cpw1_t = np.ascontiguousarray(np.clip(
        np.asarray(cp_w1, np.float32).T * 64.0, -E4MAX, E4MAX)
        .reshape(KI, P, 16)).astype(E4)
    cpw2_t = np.asarray(cp_w2, np.float32).reshape(1, 16).T.astype(BF)
    cpw2_t = np.ascontiguousarray(cpw2_t)
    cpb1_t = np.asarray(cp_b1, np.float32).reshape(16, 1)
    cpb2_t = np.asarray(cp_b2, np.float32).reshape(1, 1)
    b1_t = np.ascontiguousarray(np.asarray(b1, np.float32).reshape(KH, P).T)
    b2_t = np.ascontiguousarray(np.asarray(b2, np.float32).reshape(KH, P).T)

    in_maps = []
    for c in range(N_CORES):
        shard = x[c * BL:(c + 1) * BL]
        shT = np.ascontiguousarray(shard.T)
        xq_c = np.clip(shT[:KQ * P], -E4MAX, E4MAX).reshape(KQ, P, BL).astype(E4)
        xT = shT[KQ * P:].reshape(KB, P, BL).astype(BF)
        m = {"xq": xq_c, "xT": xT, "w1": w1_t, "w1q": w1q_t, "w2": w2_t,
             "wo": wo_t,
             "cpw1": cpw1_t, "cpw2": cpw2_t, "cpb1": cpb1_t, "cpb2": cpb2_t,
             "ident": _IDENT}
        if with_b1:
            m["b1"] = b1_t
        if with_b2:
            m["b2"] = b2_t
        in_maps.append(m)

    kw = {}
    if _trace:
        kw = dict(trace=True, tmpdir=_tmpdir or tempfile.mkdtemp(prefix="cdk_"))
    res = run_bass_kernel_spmd(nc, in_maps, list(range(N_CORES)), **kw)

    out = np.concatenate([res.results[c]["out"] for c in range(N_CORES)], axis=0)
    bo = np.asarray(bo, np.float32)
    if np.any(bo):
        out = out + bo
    if _trace:
        kernel._last_result = res
    return out


# revision 29
# speedup vs baseline: 1.5115x; 1.0009x over previous
"""Trainium2 Bass kernel for nn_ConservativeDynamicCurvatureMLP.

Data-parallel over 8 NeuronCores: the batch (8192) is sharded into 8
local shards of 1024 rows; all weights are replicated. The curvature
scalar c_avg couples the shards through a global mean, handled with a
single-scalar AllReduce.

Math (reference):
    h = tanh(x @ W1 + b1)
    u = sigmoid(h @ W2 + b2)
    c = clip(mean(MIN_C + (MAX_C-MIN_C) * sigmoid(relu(x@cp_w1.T+cp_b1)@cp_w2.T+cp_b2)), MIN_C, MAX_C)
    z = poincare_ball_layer(h, u, c, T)
    out = z @ Wo + bo

The poincare layer collapses algebraically to z = alpha(row)*h + beta(row)*u
where alpha/beta are scalar functions of the row statistics
x2=||h||^2, y2=||u||^2, xy=<h,u> and c.  The NaN fallback is measure-zero
and omitted.

Perf notes (measured on hw):
  - With 8 cores active the PE is power-throttled to ~2.0 GHz (HAM state
    31), so bf16 N=512 matmuls pace at ~263 ns.  The only lever is fewer
    PE cycles: MM2 (h@W2, the largest matmul) runs as fp8-e4m3 DoubleRow
    (256-deep contraction per pass, 2x FLOP rate, measured 216 ns/MM at
    2.4 GHz in isolation).  W2 is host-converted to e4m3 * 1024 (clipped
    to +-240, the TRN e4m3 max); the 1/1024 folds into the sigmoid's
    activation scale.  h is cast bf16->fp8 per row-tile on the DVE.
    MM1 / MMo stay bf16: fp8 there pushes rel-err too close to the 2e-2
    gate (simulated 1.96e-2 vs 1.26e-2 for MM2-only).
  - Row statistics go batch-major directly via tiny stationary-squares
    matmuls (lhsT = accumulated squares block, rhs = ones) instead of a
    feature-major ones-reduction + DRAM bounce, shortening the serial
    stats -> alpha/beta tail.  The per-row scalar chain runs once on
    [128, 8] tiles reading stats straight from SBUF.
"""

import tempfile
from contextlib import ExitStack

import numpy as np
import ml_dtypes

import concourse.bass as bass
import concourse.bacc as bacc
import concourse.mybir as mybir
import concourse.tile as tile
from concourse.bass_utils import run_bass_kernel_spmd

P = 128
N_CORES = 8
B_FULL = 8192
BL = B_FULL // N_CORES          # 1024 rows per core
IN = 3072
HID = 4096
OUT = 1000
KI = IN // P                    # 24
KQ = 12                         # leading ki-groups of MM1 done in fp8
KB = KI - KQ                    # trailing ki-groups in bf16
KH = HID // P                   # 32
NB = BL // P                    # 8 batch tiles
MIN_C = 0.001 * 0.5
MAX_C = 0.001 * 2.0
T_CONST = 0.7
EPS = 1e-7
W2S = 1024.0                    # host-side fp8 scale for W2
E4MAX = 240.0                   # TRN e4m3 saturation

dt = mybir.dt
AF = mybir.ActivationFunctionType
ALU = mybir.AluOpType
PM = mybir.MatmulPerfMode
BF = ml_dtypes.bfloat16
E4 = ml_dtypes.float8_e4m3

_nc_cache = []
_IDENT = np.eye(P, dtype=ml_dtypes.bfloat16)


def _build(with_b1, with_b2):
    nc = bacc.Bacc("TRN2", target_bir_lowering=False, debug=False,
                   num_devices=N_CORES)

    # x features 0:1024 as fp8 (partial-fp8 MM1), 1024:3072 as bf16
    xq_d = nc.dram_tensor("xq", [KQ, P, BL], dt.float8e4, kind="ExternalInput")
    xT_d = nc.dram_tensor("xT", [KB, P, BL], dt.bfloat16, kind="ExternalInput")
    # W1 scaled x1024 throughout (so fp8 and bf16 parts share one psum
    # accumulator); fp8 rows in DoubleRow pair layout, bf16 rows in halves
    w1q_d = nc.dram_tensor("w1q", [KH, P, KQ // 2, 2, P], dt.float8e4,
                           kind="ExternalInput")
    w1_d = nc.dram_tensor("w1", [KH, P, KB, P], dt.bfloat16,
                          kind="ExternalInput")
    # W2 in fp8 (x1024): w2r[mh, p, kh, q] = W2[kh*128+p, mh*128+q]
    w2_d = nc.dram_tensor("w2", [KH, P, KH, P], dt.float8e4, kind="ExternalInput")
    wo_d = nc.dram_tensor("wo", [KH, P, OUT], dt.bfloat16, kind="ExternalInput")
    cpw1_d = nc.dram_tensor("cpw1", [KI, P, 16], dt.float8e4, kind="ExternalInput")
    cpw2_d = nc.dram_tensor("cpw2", [16, 1], dt.bfloat16, kind="ExternalInput")
    cpb1_d = nc.dram_tensor("cpb1", [16, 1], dt.float32, kind="ExternalInput")
    cpb2_d = nc.dram_tensor("cpb2", [1, 1], dt.float32, kind="ExternalInput")
    ident_d = nc.dram_tensor("ident", [P, P], dt.bfloat16, kind="ExternalInput")
    b1_d = nc.dram_tensor("b1", [P, KH], dt.float32, kind="ExternalInput") if with_b1 else None
    b2_d = nc.dram_tensor("b2", [P, KH], dt.float32, kind="ExternalInput") if with_b2 else None
    out_d = nc.dram_tensor("out", [BL, OUT], dt.float32, kind="ExternalOutput")

    f32 = dt.float32
    bf16 = dt.bfloat16
    fp8 = dt.float8e4

    with tile.TileContext(nc) as tc, ExitStack() as ctx:
        const = ctx.enter_context(tc.tile_pool(name="const", bufs=1))
        big = ctx.enter_context(tc.tile_pool(name="big", bufs=1))
        htp = ctx.enter_context(tc.tile_pool(name="htp", bufs=1))
        hqp = ctx.enter_context(tc.tile_pool(name="hqp", bufs=1))
        wp = ctx.enter_context(tc.tile_pool(name="wp", bufs=2))
        wop = ctx.enter_context(tc.tile_pool(name="wop", bufs=4))
        scr = ctx.enter_context(tc.tile_pool(name="scr", bufs=2))
        zzp = ctx.enter_context(tc.tile_pool(name="zzp", bufs=3))
        sacc = ctx.enter_context(tc.tile_pool(name="sacc", bufs=1))
        abp = ctx.enter_context(tc.tile_pool(name="abp", bufs=1))
        scal = ctx.enter_context(tc.tile_pool(name="scal", bufs=1))
        cpp = ctx.enter_context(tc.tile_pool(name="cpp", bufs=1))
        dram = ctx.enter_context(tc.tile_pool(name="dram", bufs=1, space="DRAM"))

        V = nc.vector
        S = nc.scalar

        def sc(name, shape=(P, 8), dtype=f32):
            return scal.tile(list(shape), dtype, name=name, tag=name)

        # ---------- persistent activations (feature-major) ----------
        onesb = const.tile([P, 1], bf16, name="onesb")
        nc.vector.memset(onesb, 1.0)
        onesf = const.tile([P, 1], f32, name="onesf")
        nc.vector.memset(onesf, 1.0)
        # one 64KB slot holds bf16 x (ki 0:KB), fp8 x (as bitcast bytes in
        # the tail), and is later recycled as uT
        xall = big.tile([P, KH, BL], bf16, name="xall", tag="big")
        xT_sb = xall[:, 0:KB, :]
        # first weight half-row issues on the sync queue before anything else;
        # xT streams on the gpsimd + scalar queues in parallel (the load is
        # HBM-bandwidth-bound, ~35us for 6.3MB with all 8 cores pulling)
        w1q0 = wp.tile([P, KQ // 2, 2, P], fp8, name="w1qrow", tag="w1q")
        nc.sync.dma_start(out=w1q0, in_=w1q_d[0])
        xq_sb = (xall[:, KB:KB + KQ // 2, :].bitcast(fp8)
                 .rearrange("p a (t b) -> p (a t) b", t=2))
        queues = [nc.scalar, nc.gpsimd]
        for idx, (a, b) in enumerate(((0, 3), (3, 6), (6, 9), (9, 12))):
            queues[idx % 2].dma_start(
                out=xq_sb[:, a:b, :],
                in_=xq_d[a:b].rearrange("k p b -> p k b"))
        for idx, (a, b) in enumerate(((0, 2), (2, 4), (4, 6), (6, 8),
                                      (8, 10), (10, 12))):
            queues[idx % 2].dma_start(
                out=xT_sb[:, a:b, :],
                in_=xT_d[a:b].rearrange("k p b -> p k b"))
        ident_sb = const.tile([P, P], bf16, name="ident_sb")
        nc.sync.dma_start(out=ident_sb, in_=ident_d[:, :])
        hT_sb = htp.tile([P, KH, BL], bf16, name="hT_sb")
        hTq_sb = hqp.tile([P, KH, BL], fp8, name="hTq_sb")
        if with_b1:
            b1_sb = const.tile([P, KH], f32, name="b1_sb")
            nc.sync.dma_start(out=b1_sb, in_=b1_d[:, :])
        if with_b2:
            b2_sb = const.tile([P, KH], f32, name="b2_sb")
            nc.sync.dma_start(out=b2_sb, in_=b2_d[:, :])

        with ExitStack() as ph1:
            mm = ph1.enter_context(tc.tile_pool(name="mm", bufs=3, space="PSUM"))
            stp = ph1.enter_context(tc.tile_pool(name="stp", bufs=1, space="PSUM"))
            # one bank shared by the batch-major stats (cols 0:8 x2,
            # 8:16 y2, 16:24 xy) and the HAM warm-keeper matmul target
            # (cols 128:512); plus the a/b transpose target bank
            stat_ps = stp.tile([P, 512], f32, name="stat_ps")
            dum_ps = stat_ps[:, 128:512]
            abT_ps = stp.tile([16, P], bf16, name="abT_ps")

            # ---------- MM1: hT = tanh(W1.T @ xT) (bf16) ----------
            x2a = sacc.tile([P, BL], bf16, name="x2a")
            y2a = sacc.tile([P, BL], bf16, name="y2a")
            xya = sacc.tile([P, BL], bf16, name="xya")
            with nc.named_scope("mm1"):
                for mh in range(KH):
                    ps = mm.tile([P, BL], f32, name="ps", tag="mm")
                    if mh == 0:
                        w1q = w1q0
                    else:
                        w1q = wp.tile([P, KQ // 2, 2, P], fp8, name="w1qrow",
                                      tag="w1q")
                        nc.sync.dma_start(out=w1q, in_=w1q_d[mh])
                    w1r = wp.tile([P, KB, P], bf16, name="w1row", tag="w1")
                    nc.sync.dma_start(out=w1r, in_=w1_d[mh])
                    for hsl in (slice(0, 512), slice(512, BL)):
                        for j in range(KQ // 2):
                            nc.tensor.matmul(
                                ps[:, hsl], lhsT=w1q[:, j, :, :],
                                rhs=xq_sb[:, 2 * j:2 * j + 2, hsl],
                                start=(j == 0), stop=False,
                                perf_mode=PM.DoubleRow,
                                skip_group_check=True)
                        for ki in range(KB):
                            nc.tensor.matmul(ps[:, hsl], lhsT=w1r[:, ki, :],
                                             rhs=xT_sb[:, ki, hsl],
                                             start=False, stop=(ki == KB - 1),
                                             skip_group_check=True)
                    if with_b1:
                        S.activation(hT_sb[:, mh, :], ps, AF.Tanh,
                                     bias=b1_sb[:, mh:mh + 1], scale=1.0 / W2S)
                    else:
                        S.activation(hT_sb[:, mh, :], ps, AF.Tanh,
                                     scale=1.0 / W2S)
                    # fp8 copy for MM2's DoubleRow rhs
                    V.tensor_copy(hTq_sb[:, mh, :], hT_sb[:, mh, :])
                    hh = scr.tile([P, BL], bf16, name="hh", tag="hh")
                    S.activation(hh, hT_sb[:, mh, :], AF.Square)
                    if mh == 0:
                        V.tensor_copy(x2a, hh)
                    else:
                        V.tensor_add(x2a, x2a, hh)
                # x2 -> batch-major: out[b,0] = sum_p x2a[p, blk*128+b]
                for blk in range(NB):
                    nc.tensor.matmul(stat_ps[:, blk:blk + 1],
                                     lhsT=x2a[:, blk * P:(blk + 1) * P],
                                     rhs=onesb, start=True, stop=True,
                                     skip_group_check=True)

            # ---------- curvature predictor (xT still resident; the
            # AllReduce hides under MM1/MM2) ----------
            with nc.named_scope("cp"):
                cpw1_sb = const.tile([P, KI, 16], fp8, name="cpw1_sb")
                nc.sync.dma_start(out=cpw1_sb,
                                  in_=cpw1_d.rearrange("k p q -> p k q"))
                cpw2_sb = const.tile([16, 1], bf16, name="cpw2_sb")
                nc.sync.dma_start(out=cpw2_sb, in_=cpw2_d[:, :])
                cpb1_sb = const.tile([16, 1], f32, name="cpb1_sb")
                nc.sync.dma_start(out=cpb1_sb, in_=cpb1_d[:, :])
                cpb2_sb = const.tile([1, 1], f32, name="cpb2_sb")
                nc.sync.dma_start(out=cpb2_sb, in_=cpb2_d[:, :])
                cph_sb = cpp.tile([16, BL], fp8, name="cph_sb")
                for ch in range(2):
                    cps = mm.tile([16, 512], f32, name="cps", tag="mm")
                    for ki in range(KI):
                        xr = (xq_sb[:, ki, ch * 512:(ch + 1) * 512] if ki < KQ
                              else xT_sb[:, ki - KQ, ch * 512:(ch + 1) * 512])
                        nc.tensor.matmul(
                            cps, lhsT=cpw1_sb[:, ki, :], rhs=xr,
                            start=(ki == 0), stop=(ki == KI - 1))
                    S.activation(cph_sb[:, ch * 512:(ch + 1) * 512], cps,
                                 AF.Relu, bias=cpb1_sb, scale=1.0 / 64.0)
                sparts = []
                for ch in range(2):
                    c2p = mm.tile([1, 512], f32, name="c2p", tag="mm")
                    nc.tensor.matmul(c2p, lhsT=cpw2_sb,
                                     rhs=cph_sb[:16, ch * 512:(ch + 1) * 512],
                                     start=True, stop=True)
                    cpw = cpp.tile([1, 512], bf16, name="cpw", tag="cpw")
                    spart = cpp.tile([1, 1], f32, name=f"spart{ch}",
                                     tag=f"spart{ch}")
                    S.activation(cpw, c2p, AF.Sigmoid, bias=cpb2_sb,
                                 accum_out=spart)
                    sparts.append(spart)
                s_loc = cpp.tile([1, 1], f32, name="s_loc")
                V.tensor_add(s_loc, sparts[0], sparts[1])
                cin = dram.tile([1, 1], f32, name="cin")
                cout = dram.tile([1, 1], f32, name="cout")
                nc.sync.dma_start(out=cin, in_=s_loc)
                nc.gpsimd.collective_compute(
                    "AllReduce", ALU.add,
                    replica_groups=[list(range(N_CORES))],
                    ins=[cin.opt()], outs=[cout.opt()])
                s_b = sc("s_b", (P, 1))
                nc.gpsimd.dma_start(out=s_b, in_=cout.to_broadcast([P, 1]))
                # c = clip(MIN_C + (MAX_C-MIN_C)*mean(c_pred))
                c_b = sc("c_b", (P, 1))
                V.tensor_scalar(out=c_b, in0=s_b,
                                scalar1=(MAX_C - MIN_C) / B_FULL,
                                scalar2=MIN_C, op0=ALU.mult, op1=ALU.add)
                V.tensor_scalar_min(out=c_b, in0=c_b, scalar1=MAX_C)
                V.tensor_scalar_max(out=c_b, in0=c_b, scalar1=MIN_C)
                negc_b = sc("negc_b", (P, 1))
                V.tensor_scalar_mul(out=negc_b, in0=c_b, scalar1=-1.0)
                twoc_b = sc("twoc_b", (P, 1))
                V.tensor_scalar_mul(out=twoc_b, in0=c_b, scalar1=2.0)
                neg2c_b = sc("neg2c_b", (P, 1))
                V.tensor_scalar_mul(out=neg2c_b, in0=c_b, scalar1=-2.0)
                c2_b = sc("c2_b", (P, 1))
                V.tensor_mul(c2_b, c_b, c_b)

            # ---------- MM2: uT = sigmoid(W2.T @ hT) fp8 DoubleRow ----------
            uT_sb = big.tile([P, KH, BL], bf16, name="uT_sb", tag="big")
            inv_s2 = 1.0 / W2S
            with nc.named_scope("mm2"):
                for mh in range(KH):
                    ps = mm.tile([P, BL], f32, name="ps", tag="mm")
                    w2row = wp.tile([P, KH, P], fp8, name="w2row", tag="w2")
                    nc.sync.dma_start(out=w2row, in_=w2_d[mh])
                    for j in range(KH // 2):
                        nc.tensor.matmul(ps[:, 0:512],
                                         lhsT=w2row[:, 2 * j:2 * j + 2, :],
                                         rhs=hTq_sb[:, 2 * j:2 * j + 2, 0:512],
                                         start=(j == 0), stop=(j == KH // 2 - 1),
                                         perf_mode=PM.DoubleRow)
                        nc.tensor.matmul(ps[:, 512:BL],
                                         lhsT=w2row[:, 2 * j:2 * j + 2, :],
                                         rhs=hTq_sb[:, 2 * j:2 * j + 2, 512:BL],
                                         start=(j == 0), stop=(j == KH // 2 - 1),
                                         perf_mode=PM.DoubleRow)
                    if with_b2:
                        S.activation(uT_sb[:, mh, :], ps, AF.Sigmoid,
                                     bias=b2_sb[:, mh:mh + 1], scale=inv_s2)
                    else:
                        S.activation(uT_sb[:, mh, :], ps, AF.Sigmoid,
                                     scale=inv_s2)
                    uu = scr.tile([P, BL], bf16, name="uu", tag="hh")
                    S.activation(uu, uT_sb[:, mh, :], AF.Square)
                    hu = scr.tile([P, BL], bf16, name="hu", tag="hh")
                    V.tensor_mul(hu, hT_sb[:, mh, :], uT_sb[:, mh, :])
                    if mh == 0:
                        V.tensor_copy(y2a, uu)
                        V.tensor_copy(xya, hu)
                    else:
                        V.tensor_add(y2a, y2a, uu)
                        V.tensor_add(xya, xya, hu)
                # warm-keeper group 1: bridges the PE from mm2's last matmul
                # across the stats-accumulation drain (a >3.4us idle would
                # HAM-throttle the clock for the MMo matmuls).  A real
                # accumulation group so DCE cannot drop it.
                for i in range(14):
                    nc.tensor.matmul(dum_ps, lhsT=ident_sb,
                                     rhs=hT_sb[:, i, 0:384],
                                     start=(i == 0), stop=(i == 13),
                                     skip_group_check=True)
                for blk in range(NB):
                    nc.tensor.matmul(stat_ps[:, 8 + blk:9 + blk],
                                     lhsT=y2a[:, blk * P:(blk + 1) * P],
                                     rhs=onesb, start=True, stop=True,
                                     skip_group_check=True)
                    nc.tensor.matmul(stat_ps[:, 16 + blk:17 + blk],
                                     lhsT=xya[:, blk * P:(blk + 1) * P],
                                     rhs=onesb, start=True, stop=True,
                                     skip_group_check=True)

            # ---------- per-row scalar chain, batch-major [128, 8] ----------
            alpha_b = abp.tile([P, BL], bf16, name="alpha_b", tag="alpha_b")
            beta_b = abp.tile([P, BL], bf16, name="beta_b", tag="beta_b")
            ab_d = dram.tile([2, BL], bf16, name="ab_d")

            with nc.named_scope("scalars"):
                stats_sb = sc("stats_sb", (P, 24))
                V.tensor_copy(stats_sb, stat_ps[:, 0:24])
                x2 = stats_sb[:, 0:8]
                y2 = stats_sb[:, 8:16]
                xy = stats_sb[:, 16:24]
                w = sc("w")
                V.scalar_tensor_tensor(out=w, in0=xy, scalar=-2.0, in1=y2,
                                       op0=ALU.mult, op1=ALU.add)
                A1 = sc("A1")
                V.tensor_scalar(out=A1, in0=w, scalar1=c_b, scalar2=1.0,
                                op0=ALU.mult, op1=ALU.add)
                A2 = sc("A2")
                V.tensor_scalar(out=A2, in0=x2, scalar1=negc_b, scalar2=1.0,
                                op0=ALU.mult, op1=ALU.add)
                p1 = sc("p1")
                V.tensor_mul(p1, x2, y2)
                den = sc("den")
                V.tensor_scalar(out=den, in0=p1, scalar1=c2_b, scalar2=1.0,
                                op0=ALU.mult, op1=ALU.add)
                V.scalar_tensor_tensor(out=den, in0=xy, scalar=neg2c_b, in1=den,
                                       op0=ALU.mult, op1=ALU.add)
                V.tensor_scalar_add(out=den, in0=den, scalar1=EPS)
                D = sc("D")
                V.reciprocal(D, den)
                nc.tensor.matmul(stat_ps[0:1, 80:88], lhsT=onesf, rhs=D,
                                 start=True, stop=True, skip_group_check=True)
                # ||a||^2 = D^2 (A1^2 x2 - 2 A1 A2 xy + A2^2 y2)
                t1 = sc("t1")
                V.tensor_mul(t1, A1, A1)
                V.tensor_mul(t1, t1, x2)
                t2 = sc("t2")
                V.tensor_mul(t2, A1, A2)
                V.tensor_mul(t2, t2, xy)
                t3 = sc("t3")
                V.tensor_mul(t3, A2, A2)
                V.tensor_mul(t3, t3, y2)
                na2 = sc("na2")
                V.scalar_tensor_tensor(out=na2, in0=t2, scalar=-2.0, in1=t1,
                                       op0=ALU.mult, op1=ALU.add)
                V.tensor_add(na2, na2, t3)
                dsq = sc("dsq")
                V.tensor_mul(dsq, D, D)
                V.tensor_mul(na2, na2, dsq)
                # q = sqrt(c * na2) with one Newton step (ACT sqrt is low precision)
                q2 = sc("q2")
                V.tensor_scalar(out=q2, in0=na2, scalar1=c_b, scalar2=None,
                                op0=ALU.mult)
                q0 = sc("q0")
                S.activation(q0, q2, AF.Sqrt)
                V.tensor_scalar_max(out=q0, in0=q0, scalar1=1e-20)
                r0 = sc("r0")
                V.reciprocal(r0, q0)
                q = sc("q")
                V.tensor_mul(q, q2, r0)
                V.tensor_add(q, q, q0)
                V.tensor_scalar_mul(out=q, in0=q, scalar1=0.5)
                arg = sc("arg")
                V.tensor_scalar_min(out=arg, in0=q, scalar1=1.0 - 1e-5)
                # artanh(arg) = 0.5 ln((1+arg)/(1-arg)); t = tanh(T*artanh)/q
                opp = sc("opp")
                V.tensor_scalar(out=opp, in0=arg, scalar1=-1.0, scalar2=1.0,
                                op0=ALU.mult, op1=ALU.add)
                opn = sc("opn")
                V.tensor_scalar_add(out=opn, in0=arg, scalar1=1.0)
                rr = sc("rr")
                V.reciprocal(rr, opp)
                rat = sc("rat")
                V.tensor_mul(rat, opn, rr)
                lg = sc("lg")
                S.activation(lg, rat, AF.Ln)
                th = sc("th")
                S.activation(th, lg, AF.Tanh, scale=T_CONST * 0.5)
                rq = sc("rq")
                V.reciprocal(rq, q)
                tm = sc("tm")
                V.tensor_mul(tm, th, rq)
                nc.tensor.matmul(stat_ps[0:1, 88:96], lhsT=onesf, rhs=tm,
                                 start=True, stop=True, skip_group_check=True)
                # <h,a> = D (A2 xy - A1 x2)
                s1_ = sc("s1_")
                V.tensor_mul(s1_, A1, x2)
                s2_ = sc("s2_")
                V.tensor_mul(s2_, A2, xy)
                ha = sc("ha")
                V.tensor_sub(ha, s2_, s1_)
                V.tensor_mul(ha, ha, D)
                hm = sc("hm")
                V.tensor_mul(hm, tm, ha)
                tsq = sc("tsq")
                V.tensor_mul(tsq, tm, tm)
                m2 = sc("m2")
                V.tensor_mul(m2, tsq, na2)
                w2s = sc("w2s")
                V.scalar_tensor_tensor(out=w2s, in0=hm, scalar=2.0, in1=m2,
                                       op0=ALU.mult, op1=ALU.add)
                B1 = sc("B1")
                V.tensor_scalar(out=B1, in0=w2s, scalar1=c_b, scalar2=1.0,
                                op0=ALU.mult, op1=ALU.add)
                p2 = sc("p2")
                V.tensor_mul(p2, x2, m2)
                den2 = sc("den2")
                V.tensor_scalar(out=den2, in0=p2, scalar1=c2_b, scalar2=1.0,
                                op0=ALU.mult, op1=ALU.add)
                V.scalar_tensor_tensor(out=den2, in0=hm, scalar=twoc_b, in1=den2,
                                       op0=ALU.mult, op1=ALU.add)
                V.tensor_scalar_add(out=den2, in0=den2, scalar1=EPS)
                D2 = sc("D2")
                V.reciprocal(D2, den2)
                g = sc("g")
                V.tensor_mul(g, A2, tm)
                V.tensor_mul(g, g, D)
                w3 = sc("w3")
                V.tensor_mul(w3, g, A1)
                V.tensor_sub(w3, B1, w3)
                # alpha -> cols 0:8, beta -> cols 8:16 of one bf16 tile; a PE
                # transpose then yields batch-linear rows for a fast DMA
                # bounce (8 contiguous 256B descriptors instead of a 2-byte
                # element scatter).
                ab_bm = sc("ab_bm", (P, 16), bf16)
                V.tensor_mul(ab_bm[:, 0:8], w3, D2)
                w4 = sc("w4")
                V.tensor_mul(w4, g, A2)
                V.tensor_mul(ab_bm[:, 8:16], w4, D2)
                nc.tensor.transpose(abT_ps, ab_bm, ident_sb)
                # warm-keeper taps pinned after the chain end: the scheduler
                # hoists dependency-free matmuls, so these read ab_bm
                for i in range(3):
                    nc.tensor.matmul(stat_ps[:, 32 + 16 * i:48 + 16 * i],
                                     lhsT=ident_sb, rhs=ab_bm,
                                     start=True, stop=True,
                                     skip_group_check=True)
                abT_sb = sc("abT_sb", (16, P), bf16)
                V.tensor_copy(abT_sb, abT_ps)
                nc.sync.dma_start(out=ab_d[0, :].rearrange("(j b) -> j b", j=8),
                                  in_=abT_sb[0:8, :])
                nc.sync.dma_start(out=ab_d[1, :].rearrange("(j b) -> j b", j=8),
                                  in_=abT_sb[8:16, :])
                for hsl in (slice(0, 512), slice(512, BL)):
                    nc.scalar.dma_start(out=alpha_b[:, hsl],
                                        in_=ab_d[0:1, hsl].to_broadcast([P, 512]))
                    nc.sync.dma_start(out=beta_b[:, hsl],
                                      in_=ab_d[1:2, hsl].to_broadcast([P, 512]))
                for i in range(2):
                    nc.tensor.matmul(dum_ps, lhsT=ident_sb,
                                     rhs=(alpha_b[:, 0:384] if i == 0
                                          else beta_b[:, 512:896]),
                                     start=(i == 0), stop=(i == 1),
                                     skip_group_check=True)
                # anchor all warm-keeper matmuls against DCE
                dum_sb = sc("dum_sb", (1, 128), bf16)
                V.tensor_copy(dum_sb, stat_ps[0:1, 24:152])
                dum_d = dram.tile([1, 128], bf16, name="dum_d")
                nc.gpsimd.dma_start(out=dum_d, in_=dum_sb)

        # psum pools (mm, stp) released here

        # ---------- z = alpha*h + beta*u (overwrites uT in place),
        # then out = z @ Wo.  Processed in two batch-column halves so the
        # MMo matmuls of half 0 overlap the DVE z-combine of half 1.
        with ExitStack() as ph2:
            mmo = ph2.enter_context(tc.tile_pool(name="mmo", bufs=8,
                                                 space="PSUM"))
            # z-combines for BOTH halves first, so the DVE never head-of-line
            # blocks mmo1's z behind bg0's psum drains
            for bg in range(2):
                csl = slice(bg * 512, (bg + 1) * 512)
                with nc.named_scope(f"zcomb{bg}"):
                    for kh in range(KH):
                        t1z = zzp.tile([P, 512], bf16, name="t1z", tag="zz")
                        V.tensor_mul(t1z, hT_sb[:, kh, csl], alpha_b[:, csl])
                        t2z = zzp.tile([P, 512], bf16, name="t2z", tag="zz")
                        V.tensor_mul(t2z, uT_sb[:, kh, csl], beta_b[:, csl])
                        V.tensor_add(uT_sb[:, kh, csl], t1z, t2z)
            for bg in range(2):
                with nc.named_scope(f"mmo{bg}"):
                    pso = [mmo.tile([P, 500], f32, name=f"pso{bg}_{i}",
                                    tag="mmo") for i in range(8)]
                    for kh in range(KH):
                        wot = wop.tile([P, OUT], bf16, name="wot", tag="wo")
                        nc.sync.dma_start(out=wot, in_=wo_d[kh])
                        for i in range(4):
                            b = bg * 4 + i
                            for och in range(2):
                                nc.tensor.matmul(
                                    pso[i * 2 + och],
                                    lhsT=uT_sb[:, kh, b * P:(b + 1) * P],
                                    rhs=wot[:, och * 500:(och + 1) * 500],
                                    start=(kh == 0), stop=(kh == KH - 1))
                    for i in range(4):
                        b = bg * 4 + i
                        for och in range(2):
                            # stage drains across the scr ring and the (now
                            # dead) alpha/beta tiles for 4-deep DMA pipelining
                            slot = (i * 2 + och) % 3
                            if slot == 0:
                                ob = scr.tile([P, 500], f32, name="ob",
                                              tag="hh")
                            elif slot == 1:
                                ob = abp.tile([P, 500], f32, name="ob",
                                              tag="alpha_b")
                            else:
                                ob = abp.tile([P, 500], f32, name="ob",
                                              tag="beta_b")
                            # bg0 drains stay off the DVE (it is mid-zcomb1
                            # and would head-of-line block); ACT frees banks
                            if bg == 0 or och == 0:
                                S.copy(ob, pso[i * 2 + och])
                            else:
                                V.tensor_copy(ob, pso[i * 2 + 1])
                            outq = nc.sync if och == 0 else nc.scalar
                            outq.dma_start(
                                out=out_d[b * P:(b + 1) * P,
                                          och * 500:(och + 1) * 500],
                                in_=ob)

    nc.compile()
    return nc


def _get_nc(with_b1, with_b2):
    for k, v in _nc_cache:
        if k == (with_b1, with_b2):
            return v
    nc = _build(with_b1, with_b2)
    _nc_cache.append(((with_b1, with_b2), nc))
    return nc


def kernel(x, W1, b1, W2, b2, Wo, bo, cp_w1, cp_b1, cp_w2, cp_b2,
           _trace=False, _tmpdir=None):
    x = np.asarray(x, dtype=np.float32)
    with_b1 = bool(np.any(b1))
    with_b2 = bool(np.any(b2))
    nc = _get_nc(with_b1, with_b2)

    # W1 pre-scaled x1024 so fp8 and bf16 parts share one accumulator
    w1s = np.asarray(W1, np.float32) * W2S
    # fp8 rows 0:1024 in DoubleRow pair layout [KH, P, KQ//2, 2, P]
    w1q_t = np.ascontiguousarray(
        np.clip(w1s[:KQ * P], -E4MAX, E4MAX)
        .reshape(KQ, P, KH, P).transpose(2, 1, 0, 3)
    ).astype(E4).reshape(KH, P, KQ // 2, 2, P)
    # bf16 rows [KH, P, KB, P]
    w1_t = np.ascontiguousarray(
        w1s[KQ * P:].reshape(KB, P, KH, P).transpose(2, 1, 0, 3)
    ).astype(BF)
    # w2r[mh, p, kh, q] = W2[kh*128+p, mh*128+q], fp8 e4m3 scaled x1024
    w2_t = np.ascontiguousarray(
        np.clip(np.asarray(W2, np.float32) * W2S, -E4MAX, E4MAX)
        .reshape(KH, P, KH, P).transpose(2, 1, 0, 3)
    ).astype(E4)
    wo_t = np.asarray(Wo, np.float32).reshape(KH, P, OUT).astype(BF)
    cpw1_t = np.ascontiguousarray(np.clip(
        np.asarray(cp_w1, np.float32).T * 64.0, -E4MAX, E4MAX)
        .reshape(KI, P, 16)).astype(E4)
    cpw2_t = np.asarray(cp_w2, np.float32).reshape(1, 16).T.astype(BF)
    cpw2_t = np.ascontiguousarray(cpw2_t)
    cpb1_t = np.asarray(cp_b1, np.float32).reshape(16, 1)
    cpb2_t = np.asarray(cp_b2, np.float32).reshape(1, 1)
    b1_t = np.ascontiguousarray(np.asarray(b1, np.float32).reshape(KH, P).T)
    b2_t = np.ascontiguousarray(np.asarray(b2, np.float32).reshape(KH, P).T)

    in_maps = []
    for c in range(N_CORES):
        shard = x[c * BL:(c + 1) * BL]
        shT = np.ascontiguousarray(shard.T)
        xq_c = np.clip(shT[:KQ * P], -E4MAX, E4MAX).reshape(KQ, P, BL).astype(E4)
        xT = shT[KQ * P:].reshape(KB, P, BL).astype(BF)
        m = {"xq": xq_c, "xT": xT, "w1": w1_t, "w1q": w1q_t, "w2": w2_t,
             "wo": wo_t,
             "cpw1": cpw1_t, "cpw2": cpw2_t, "cpb1": cpb1_t, "cpb2": cpb2_t,
             "ident": _IDENT}
        if with_b1:
            m["b1"] = b1_t
        if with_b2:
            m["b2"] = b2_t
        in_maps.append(m)

    kw = {}
    if _trace:
        kw = dict(trace=True, tmpdir=_tmpdir or tempfile.mkdtemp(prefix="cdk_"))
    res = run_bass_kernel_spmd(nc, in_maps, list(range(N_CORES)), **kw)

    out = np.concatenate([res.results[c]["out"] for c in range(N_CORES)], axis=0)
    bo = np.asarray(bo, np.float32)
    if np.any(bo):
        out = out + bo
    if _trace:
        kernel._last_result = res
    return out


# revision 30
# speedup vs baseline: 1.5171x; 1.0037x over previous
"""Trainium2 Bass kernel for nn_ConservativeDynamicCurvatureMLP.

Data-parallel over 8 NeuronCores: the batch (8192) is sharded into 8
local shards of 1024 rows; all weights are replicated. The curvature
scalar c_avg couples the shards through a global mean, handled with a
single-scalar AllReduce.

Math (reference):
    h = tanh(x @ W1 + b1)
    u = sigmoid(h @ W2 + b2)
    c = clip(mean(MIN_C + (MAX_C-MIN_C) * sigmoid(relu(x@cp_w1.T+cp_b1)@cp_w2.T+cp_b2)), MIN_C, MAX_C)
    z = poincare_ball_layer(h, u, c, T)
    out = z @ Wo + bo

The poincare layer collapses algebraically to z = alpha(row)*h + beta(row)*u
where alpha/beta are scalar functions of the row statistics
x2=||h||^2, y2=||u||^2, xy=<h,u> and c.  The NaN fallback is measure-zero
and omitted.

Perf notes (measured on hw):
  - With 8 cores active the PE is power-throttled to ~2.0 GHz (HAM state
    31), so bf16 N=512 matmuls pace at ~263 ns.  The only lever is fewer
    PE cycles: MM2 (h@W2, the largest matmul) runs as fp8-e4m3 DoubleRow
    (256-deep contraction per pass, 2x FLOP rate, measured 216 ns/MM at
    2.4 GHz in isolation).  W2 is host-converted to e4m3 * 1024 (clipped
    to +-240, the TRN e4m3 max); the 1/1024 folds into the sigmoid's
    activation scale.  h is cast bf16->fp8 per row-tile on the DVE.
    MM1 / MMo stay bf16: fp8 there pushes rel-err too close to the 2e-2
    gate (simulated 1.96e-2 vs 1.26e-2 for MM2-only).
  - Row statistics go batch-major directly via tiny stationary-squares
    matmuls (lhsT = accumulated squares block, rhs = ones) instead of a
    feature-major ones-reduction + DRAM bounce, shortening the serial
    stats -> alpha/beta tail.  The per-row scalar chain runs once on
    [128, 8] tiles reading stats straight from SBUF.
"""

import tempfile
from contextlib import ExitStack

import numpy as np
import ml_dtypes

import concourse.bass as bass
import concourse.bacc as bacc
import concourse.mybir as mybir
import concourse.tile as tile
from concourse.bass_utils import run_bass_kernel_spmd

P = 128
N_CORES = 8
B_FULL = 8192
BL = B_FULL // N_CORES          # 1024 rows per core
IN = 3072
HID = 4096
OUT = 1000
KI = IN // P                    # 24
KQ = 12                         # leading ki-groups of MM1 done in fp8
KB = KI - KQ                    # trailing ki-groups in bf16
KH = HID // P                   # 32
NB = BL // P                    # 8 batch tiles
MIN_C = 0.001 * 0.5
MAX_C = 0.001 * 2.0
T_CONST = 0.7
EPS = 1e-7
W2S = 1024.0                    # host-side fp8 scale for W2
E4MAX = 240.0                   # TRN e4m3 saturation

dt = mybir.dt
AF = mybir.ActivationFunctionType
ALU = mybir.AluOpType
PM = mybir.MatmulPerfMode
BF = ml_dtypes.bfloat16
E4 = ml_dtypes.float8_e4m3

_nc_cache = []
_IDENT = np.eye(P, dtype=ml_dtypes.bfloat16)


def _build(with_b1, with_b2):
    nc = bacc.Bacc("TRN2", target_bir_lowering=False, debug=False,
                   num_devices=N_CORES)

    # x features 0:1024 as fp8 (partial-fp8 MM1), 1024:3072 as bf16
    xq_d = nc.dram_tensor("xq", [KQ, P, BL], dt.float8e4, kind="ExternalInput")
    xT_d = nc.dram_tensor("xT", [KB, P, BL], dt.bfloat16, kind="ExternalInput")
    # W1 scaled x1024 throughout (so fp8 and bf16 parts share one psum
    # accumulator); fp8 rows in DoubleRow pair layout, bf16 rows in halves
    w1q_d = nc.dram_tensor("w1q", [KH, P, KQ // 2, 2, P], dt.float8e4,
                           kind="ExternalInput")
    w1_d = nc.dram_tensor("w1", [KH, P, KB, P], dt.bfloat16,
                          kind="ExternalInput")
    # W2 in fp8 (x1024): w2r[mh, p, kh, q] = W2[kh*128+p, mh*128+q]
    w2_d = nc.dram_tensor("w2", [KH, P, KH, P], dt.float8e4, kind="ExternalInput")
    wo_d = nc.dram_tensor("wo", [KH, P, OUT], dt.bfloat16, kind="ExternalInput")
    cpw1_d = nc.dram_tensor("cpw1", [KI, P, 16], dt.float8e4, kind="ExternalInput")
    cpw2_d = nc.dram_tensor("cpw2", [16, 1], dt.bfloat16, kind="ExternalInput")
    cpb1_d = nc.dram_tensor("cpb1", [16, 1], dt.float32, kind="ExternalInput")
    cpb2_d = nc.dram_tensor("cpb2", [1, 1], dt.float32, kind="ExternalInput")
    ident_d = nc.dram_tensor("ident", [P, P], dt.bfloat16, kind="ExternalInput")
    b1_d = nc.dram_tensor("b1", [P, KH], dt.float32, kind="ExternalInput") if with_b1 else None
    b2_d = nc.dram_tensor("b2", [P, KH], dt.float32, kind="ExternalInput") if with_b2 else None
    out_d = nc.dram_tensor("out", [BL, OUT], dt.float32, kind="ExternalOutput")

    f32 = dt.float32
    bf16 = dt.bfloat16
    fp8 = dt.float8e4

    with tile.TileContext(nc) as tc, ExitStack() as ctx:
        const = ctx.enter_context(tc.tile_pool(name="const", bufs=1))
        big = ctx.enter_context(tc.tile_pool(name="big", bufs=1))
        htp = ctx.enter_context(tc.tile_pool(name="htp", bufs=1))
        hqp = ctx.enter_context(tc.tile_pool(name="hqp", bufs=1))
        wp = ctx.enter_context(tc.tile_pool(name="wp", bufs=2))
        wop = ctx.enter_context(tc.tile_pool(name="wop", bufs=4))
        scr = ctx.enter_context(tc.tile_pool(name="scr", bufs=2))
        zzp = ctx.enter_context(tc.tile_pool(name="zzp", bufs=3))
        sacc = ctx.enter_context(tc.tile_pool(name="sacc", bufs=1))
        abp = ctx.enter_context(tc.tile_pool(name="abp", bufs=1))
        scal = ctx.enter_context(tc.tile_pool(name="scal", bufs=1))
        cpp = ctx.enter_context(tc.tile_pool(name="cpp", bufs=1))
        dram = ctx.enter_context(tc.tile_pool(name="dram", bufs=1, space="DRAM"))

        V = nc.vector
        S = nc.scalar

        def sc(name, shape=(P, 8), dtype=f32):
            return scal.tile(list(shape), dtype, name=name, tag=name)

        # ---------- persistent activations (feature-major) ----------
        onesb = const.tile([P, 1], bf16, name="onesb")
        nc.vector.memset(onesb, 1.0)
        onesf = const.tile([P, 1], f32, name="onesf")
        nc.vector.memset(onesf, 1.0)
        # one 64KB slot holds bf16 x (ki 0:KB), fp8 x (as bitcast bytes in
        # the tail), and is later recycled as uT
        xall = big.tile([P, KH, BL], bf16, name="xall", tag="big")
        xT_sb = xall[:, 0:KB, :]
        # first weight half-row issues on the sync queue before anything else;
        # xT streams on the gpsimd + scalar queues in parallel (the load is
        # HBM-bandwidth-bound, ~35us for 6.3MB with all 8 cores pulling)
        w1q0 = wp.tile([P, KQ // 2, 2, P], fp8, name="w1qrow", tag="w1q")
        nc.sync.dma_start(out=w1q0, in_=w1q_d[0])
        xq_sb = (xall[:, KB:KB + KQ // 2, :].bitcast(fp8)
                 .rearrange("p a (t b) -> p (a t) b", t=2))
        queues = [nc.scalar, nc.gpsimd]
        for idx, (a, b) in enumerate(((0, 2), (2, 4), (4, 6), (6, 9), (9, 12))):
            queues[idx % 2].dma_start(
                out=xq_sb[:, a:b, :],
                in_=xq_d[a:b].rearrange("k p b -> p k b"))
        for idx, (a, b) in enumerate(((0, 2), (2, 4), (4, 6), (6, 8),
                                      (8, 10), (10, 12))):
            queues[idx % 2].dma_start(
                out=xT_sb[:, a:b, :],
                in_=xT_d[a:b].rearrange("k p b -> p k b"))
        ident_sb = const.tile([P, P], bf16, name="ident_sb")
        nc.sync.dma_start(out=ident_sb, in_=ident_d[:, :])
        hT_sb = htp.tile([P, KH, BL], bf16, name="hT_sb")
        hTq_sb = hqp.tile([P, KH, BL], fp8, name="hTq_sb")
        if with_b1:
            b1_sb = const.tile([P, KH], f32, name="b1_sb")
            nc.sync.dma_start(out=b1_sb, in_=b1_d[:, :])
        if with_b2:
            b2_sb = const.tile([P, KH], f32, name="b2_sb")
            nc.sync.dma_start(out=b2_sb, in_=b2_d[:, :])

        with ExitStack() as ph1:
            mm = ph1.enter_context(tc.tile_pool(name="mm", bufs=3, space="PSUM"))
            stp = ph1.enter_context(tc.tile_pool(name="stp", bufs=1, space="PSUM"))
            # one bank shared by the batch-major stats (cols 0:8 x2,
            # 8:16 y2, 16:24 xy) and the HAM warm-keeper matmul target
            # (cols 128:512); plus the a/b transpose target bank
            stat_ps = stp.tile([P, 512], f32, name="stat_ps")
            dum_ps = stat_ps[:, 128:512]
            abT_ps = stp.tile([16, P], bf16, name="abT_ps")

            # ---------- MM1: hT = tanh(W1.T @ xT) (bf16) ----------
            x2a = sacc.tile([P, BL], bf16, name="x2a")
            y2a = sacc.tile([P, BL], bf16, name="y2a")
            xya = sacc.tile([P, BL], bf16, name="xya")
            with nc.named_scope("mm1"):
                for mh in range(KH):
                    ps = mm.tile([P, BL], f32, name="ps", tag="mm")
                    if mh == 0:
                        w1q = w1q0
                    else:
                        w1q = wp.tile([P, KQ // 2, 2, P], fp8, name="w1qrow",
                                      tag="w1q")
                        nc.sync.dma_start(out=w1q, in_=w1q_d[mh])
                    w1r = wp.tile([P, KB, P], bf16, name="w1row", tag="w1")
                    nc.sync.dma_start(out=w1r, in_=w1_d[mh])
                    for hsl in (slice(0, 512), slice(512, BL)):
                        for j in range(KQ // 2):
                            nc.tensor.matmul(
                                ps[:, hsl], lhsT=w1q[:, j, :, :],
                                rhs=xq_sb[:, 2 * j:2 * j + 2, hsl],
                                start=(j == 0), stop=False,
                                perf_mode=PM.DoubleRow,
                                skip_group_check=True)
                        for ki in range(KB):
                            nc.tensor.matmul(ps[:, hsl], lhsT=w1r[:, ki, :],
                                             rhs=xT_sb[:, ki, hsl],
                                             start=False, stop=(ki == KB - 1),
                                             skip_group_check=True)
                    if with_b1:
                        S.activation(hT_sb[:, mh, :], ps, AF.Tanh,
                                     bias=b1_sb[:, mh:mh + 1], scale=1.0 / W2S)
                    else:
                        S.activation(hT_sb[:, mh, :], ps, AF.Tanh,
                                     scale=1.0 / W2S)
                    # fp8 copy for MM2's DoubleRow rhs
                    V.tensor_copy(hTq_sb[:, mh, :], hT_sb[:, mh, :])
                    hh = scr.tile([P, BL], bf16, name="hh", tag="hh")
                    S.activation(hh, hT_sb[:, mh, :], AF.Square)
                    if mh == 0:
                        V.tensor_copy(x2a, hh)
                    else:
                        V.tensor_add(x2a, x2a, hh)
                # x2 -> batch-major: out[b,0] = sum_p x2a[p, blk*128+b]
                for blk in range(NB):
                    nc.tensor.matmul(stat_ps[:, blk:blk + 1],
                                     lhsT=x2a[:, blk * P:(blk + 1) * P],
                                     rhs=onesb, start=True, stop=True,
                                     skip_group_check=True)

            # ---------- curvature predictor (xT still resident; the
            # AllReduce hides under MM1/MM2) ----------
            with nc.named_scope("cp"):
                cpw1_sb = const.tile([P, KI, 16], fp8, name="cpw1_sb")
                nc.sync.dma_start(out=cpw1_sb,
                                  in_=cpw1_d.rearrange("k p q -> p k q"))
                cpw2_sb = const.tile([16, 1], bf16, name="cpw2_sb")
                nc.sync.dma_start(out=cpw2_sb, in_=cpw2_d[:, :])
                cpb1_sb = const.tile([16, 1], f32, name="cpb1_sb")
                nc.sync.dma_start(out=cpb1_sb, in_=cpb1_d[:, :])
                cpb2_sb = const.tile([1, 1], f32, name="cpb2_sb")
                nc.sync.dma_start(out=cpb2_sb, in_=cpb2_d[:, :])
                cph_sb = cpp.tile([16, BL], fp8, name="cph_sb")
                for ch in range(2):
                    cps = mm.tile([16, 512], f32, name="cps", tag="mm")
                    for ki in range(KI):
                        xr = (xq_sb[:, ki, ch * 512:(ch + 1) * 512] if ki < KQ
                              else xT_sb[:, ki - KQ, ch * 512:(ch + 1) * 512])
                        nc.tensor.matmul(
                            cps, lhsT=cpw1_sb[:, ki, :], rhs=xr,
                            start=(ki == 0), stop=(ki == KI - 1))
                    S.activation(cph_sb[:, ch * 512:(ch + 1) * 512], cps,
                                 AF.Relu, bias=cpb1_sb, scale=1.0 / 64.0)
                sparts = []
                for ch in range(2):
                    c2p = mm.tile([1, 512], f32, name="c2p", tag="mm")
                    nc.tensor.matmul(c2p, lhsT=cpw2_sb,
                                     rhs=cph_sb[:16, ch * 512:(ch + 1) * 512],
                                     start=True, stop=True)
                    cpw = cpp.tile([1, 512], bf16, name="cpw", tag="cpw")
                    spart = cpp.tile([1, 1], f32, name=f"spart{ch}",
                                     tag=f"spart{ch}")
                    S.activation(cpw, c2p, AF.Sigmoid, bias=cpb2_sb,
                                 accum_out=spart)
                    sparts.append(spart)
                s_loc = cpp.tile([1, 1], f32, name="s_loc")
                V.tensor_add(s_loc, sparts[0], sparts[1])
                cin = dram.tile([1, 1], f32, name="cin")
                cout = dram.tile([1, 1], f32, name="cout")
                nc.sync.dma_start(out=cin, in_=s_loc)
                nc.gpsimd.collective_compute(
                    "AllReduce", ALU.add,
                    replica_groups=[list(range(N_CORES))],
                    ins=[cin.opt()], outs=[cout.opt()])
                s_b = sc("s_b", (P, 1))
                nc.gpsimd.dma_start(out=s_b, in_=cout.to_broadcast([P, 1]))
                # c = clip(MIN_C + (MAX_C-MIN_C)*mean(c_pred))
                c_b = sc("c_b", (P, 1))
                V.tensor_scalar(out=c_b, in0=s_b,
                                scalar1=(MAX_C - MIN_C) / B_FULL,
                                scalar2=MIN_C, op0=ALU.mult, op1=ALU.add)
                V.tensor_scalar_min(out=c_b, in0=c_b, scalar1=MAX_C)
                V.tensor_scalar_max(out=c_b, in0=c_b, scalar1=MIN_C)
                negc_b = sc("negc_b", (P, 1))
                V.tensor_scalar_mul(out=negc_b, in0=c_b, scalar1=-1.0)
                twoc_b = sc("twoc_b", (P, 1))
                V.tensor_scalar_mul(out=twoc_b, in0=c_b, scalar1=2.0)
                neg2c_b = sc("neg2c_b", (P, 1))
                V.tensor_scalar_mul(out=neg2c_b, in0=c_b, scalar1=-2.0)
                c2_b = sc("c2_b", (P, 1))
                V.tensor_mul(c2_b, c_b, c_b)

            # ---------- MM2: uT = sigmoid(W2.T @ hT) fp8 DoubleRow ----------
            uT_sb = big.tile([P, KH, BL], bf16, name="uT_sb", tag="big")
            inv_s2 = 1.0 / W2S
            with nc.named_scope("mm2"):
                for mh in range(KH):
                    ps = mm.tile([P, BL], f32, name="ps", tag="mm")
                    w2row = wp.tile([P, KH, P], fp8, name="w2row", tag="w2")
                    nc.sync.dma_start(out=w2row, in_=w2_d[mh])
                    for j in range(KH // 2):
                        nc.tensor.matmul(ps[:, 0:512],
                                         lhsT=w2row[:, 2 * j:2 * j + 2, :],
                                         rhs=hTq_sb[:, 2 * j:2 * j + 2, 0:512],
                                         start=(j == 0), stop=(j == KH // 2 - 1),
                                         perf_mode=PM.DoubleRow)
                        nc.tensor.matmul(ps[:, 512:BL],
                                         lhsT=w2row[:, 2 * j:2 * j + 2, :],
                                         rhs=hTq_sb[:, 2 * j:2 * j + 2, 512:BL],
                                         start=(j == 0), stop=(j == KH // 2 - 1),
                                         perf_mode=PM.DoubleRow)
                    if with_b2:
                        S.activation(uT_sb[:, mh, :], ps, AF.Sigmoid,
                                     bias=b2_sb[:, mh:mh + 1], scale=inv_s2)
                    else:
                        S.activation(uT_sb[:, mh, :], ps, AF.Sigmoid,
                                     scale=inv_s2)
                    uu = scr.tile([P, BL], bf16, name="uu", tag="hh")
                    S.activation(uu, uT_sb[:, mh, :], AF.Square)
                    hu = scr.tile([P, BL], bf16, name="hu", tag="hh")
                    V.tensor_mul(hu, hT_sb[:, mh, :], uT_sb[:, mh, :])
                    if mh == 0:
                        V.tensor_copy(y2a, uu)
                        V.tensor_copy(xya, hu)
                    else:
                        V.tensor_add(y2a, y2a, uu)
                        V.tensor_add(xya, xya, hu)
                # warm-keeper group 1: bridges the PE from mm2's last matmul
                # across the stats-accumulation drain (a >3.4us idle would
                # HAM-throttle the clock for the MMo matmuls).  A real
                # accumulation group so DCE cannot drop it.
                for i in range(18):
                    nc.tensor.matmul(dum_ps, lhsT=ident_sb,
                                     rhs=hT_sb[:, i, 0:384],
                                     start=(i == 0), stop=(i == 17),
                                     skip_group_check=True)
                for blk in range(NB):
                    nc.tensor.matmul(stat_ps[:, 8 + blk:9 + blk],
                                     lhsT=y2a[:, blk * P:(blk + 1) * P],
                                     rhs=onesb, start=True, stop=True,
                                     skip_group_check=True)
                    nc.tensor.matmul(stat_ps[:, 16 + blk:17 + blk],
                                     lhsT=xya[:, blk * P:(blk + 1) * P],
                                     rhs=onesb, start=True, stop=True,
                                     skip_group_check=True)

            # ---------- per-row scalar chain, batch-major [128, 8] ----------
            alpha_b = abp.tile([P, BL], bf16, name="alpha_b", tag="alpha_b")
            beta_b = abp.tile([P, BL], bf16, name="beta_b", tag="beta_b")
            ab_d = dram.tile([2, BL], bf16, name="ab_d")

            with nc.named_scope("scalars"):
                stats_sb = sc("stats_sb", (P, 24))
                V.tensor_copy(stats_sb, stat_ps[:, 0:24])
                x2 = stats_sb[:, 0:8]
                y2 = stats_sb[:, 8:16]
                xy = stats_sb[:, 16:24]
                w = sc("w")
                V.scalar_tensor_tensor(out=w, in0=xy, scalar=-2.0, in1=y2,
                                       op0=ALU.mult, op1=ALU.add)
                A1 = sc("A1")
                V.tensor_scalar(out=A1, in0=w, scalar1=c_b, scalar2=1.0,
                                op0=ALU.mult, op1=ALU.add)
                A2 = sc("A2")
                V.tensor_scalar(out=A2, in0=x2, scalar1=negc_b, scalar2=1.0,
                                op0=ALU.mult, op1=ALU.add)
                p1 = sc("p1")
                V.tensor_mul(p1, x2, y2)
                den = sc("den")
                V.tensor_scalar(out=den, in0=p1, scalar1=c2_b, scalar2=1.0,
                                op0=ALU.mult, op1=ALU.add)
                V.scalar_tensor_tensor(out=den, in0=xy, scalar=neg2c_b, in1=den,
                                       op0=ALU.mult, op1=ALU.add)
                V.tensor_scalar_add(out=den, in0=den, scalar1=EPS)
                D = sc("D")
                V.reciprocal(D, den)
                nc.tensor.matmul(stat_ps[0:1, 80:88], lhsT=onesf, rhs=D,
                                 start=True, stop=True, skip_group_check=True)
                # ||a||^2 = D^2 (A1^2 x2 - 2 A1 A2 xy + A2^2 y2)
                t1 = sc("t1")
                V.tensor_mul(t1, A1, A1)
                V.tensor_mul(t1, t1, x2)
                t2 = sc("t2")
                V.tensor_mul(t2, A1, A2)
                V.tensor_mul(t2, t2, xy)
                t3 = sc("t3")
                V.tensor_mul(t3, A2, A2)
                V.tensor_mul(t3, t3, y2)
                na2 = sc("na2")
                V.scalar_tensor_tensor(out=na2, in0=t2, scalar=-2.0, in1=t1,
                                       op0=ALU.mult, op1=ALU.add)
                V.tensor_add(na2, na2, t3)
                dsq = sc("dsq")
                V.tensor_mul(dsq, D, D)
                V.tensor_mul(na2, na2, dsq)
                # q = sqrt(c * na2) with one Newton step (ACT sqrt is low precision)
                q2 = sc("q2")
                V.tensor_scalar(out=q2, in0=na2, scalar1=c_b, scalar2=None,
                                op0=ALU.mult)
                q0 = sc("q0")
                S.activation(q0, q2, AF.Sqrt)
                V.tensor_scalar_max(out=q0, in0=q0, scalar1=1e-20)
                nc.tensor.matmul(stat_ps[0:1, 96:104], lhsT=onesf, rhs=q0,
                                 start=True, stop=True, skip_group_check=True)
                r0 = sc("r0")
                V.reciprocal(r0, q0)
                q = sc("q")
                V.tensor_mul(q, q2, r0)
                V.tensor_add(q, q, q0)
                V.tensor_scalar_mul(out=q, in0=q, scalar1=0.5)
                arg = sc("arg")
                V.tensor_scalar_min(out=arg, in0=q, scalar1=1.0 - 1e-5)
                # artanh(arg) = 0.5 ln((1+arg)/(1-arg)); t = tanh(T*artanh)/q
                opp = sc("opp")
                V.tensor_scalar(out=opp, in0=arg, scalar1=-1.0, scalar2=1.0,
                                op0=ALU.mult, op1=ALU.add)
                opn = sc("opn")
                V.tensor_scalar_add(out=opn, in0=arg, scalar1=1.0)
                rr = sc("rr")
                V.reciprocal(rr, opp)
                rat = sc("rat")
                V.tensor_mul(rat, opn, rr)
                lg = sc("lg")
                S.activation(lg, rat, AF.Ln)
                th = sc("th")
                S.activation(th, lg, AF.Tanh, scale=T_CONST * 0.5)
                rq = sc("rq")
                V.reciprocal(rq, q)
                tm = sc("tm")
                V.tensor_mul(tm, th, rq)
                nc.tensor.matmul(stat_ps[0:1, 88:96], lhsT=onesf, rhs=tm,
                                 start=True, stop=True, skip_group_check=True)
                # <h,a> = D (A2 xy - A1 x2)
                s1_ = sc("s1_")
                V.tensor_mul(s1_, A1, x2)
                s2_ = sc("s2_")
                V.tensor_mul(s2_, A2, xy)
                ha = sc("ha")
                V.tensor_sub(ha, s2_, s1_)
                V.tensor_mul(ha, ha, D)
                nc.tensor.matmul(stat_ps[0:1, 104:112], lhsT=onesf, rhs=ha,
                                 start=True, stop=True, skip_group_check=True)
                hm = sc("hm")
                V.tensor_mul(hm, tm, ha)
                tsq = sc("tsq")
                V.tensor_mul(tsq, tm, tm)
                m2 = sc("m2")
                V.tensor_mul(m2, tsq, na2)
                w2s = sc("w2s")
                V.scalar_tensor_tensor(out=w2s, in0=hm, scalar=2.0, in1=m2,
                                       op0=ALU.mult, op1=ALU.add)
                B1 = sc("B1")
                V.tensor_scalar(out=B1, in0=w2s, scalar1=c_b, scalar2=1.0,
                                op0=ALU.mult, op1=ALU.add)
                p2 = sc("p2")
                V.tensor_mul(p2, x2, m2)
                den2 = sc("den2")
                V.tensor_scalar(out=den2, in0=p2, scalar1=c2_b, scalar2=1.0,
                                op0=ALU.mult, op1=ALU.add)
                V.scalar_tensor_tensor(out=den2, in0=hm, scalar=twoc_b, in1=den2,
                                       op0=ALU.mult, op1=ALU.add)
                V.tensor_scalar_add(out=den2, in0=den2, scalar1=EPS)
                D2 = sc("D2")
                V.reciprocal(D2, den2)
                g = sc("g")
                V.tensor_mul(g, A2, tm)
                V.tensor_mul(g, g, D)
                w3 = sc("w3")
                V.tensor_mul(w3, g, A1)
                V.tensor_sub(w3, B1, w3)
                # alpha -> cols 0:8, beta -> cols 8:16 of one bf16 tile; a PE
                # transpose then yields batch-linear rows for a fast DMA
                # bounce (8 contiguous 256B descriptors instead of a 2-byte
                # element scatter).
                ab_bm = sc("ab_bm", (P, 16), bf16)
                V.tensor_mul(ab_bm[:, 0:8], w3, D2)
                w4 = sc("w4")
                V.tensor_mul(w4, g, A2)
                V.tensor_mul(ab_bm[:, 8:16], w4, D2)
                nc.tensor.transpose(abT_ps, ab_bm, ident_sb)
                # warm-keeper taps pinned after the chain end: the scheduler
                # hoists dependency-free matmuls, so these read ab_bm
                for i in range(3):
                    nc.tensor.matmul(stat_ps[:, 32 + 16 * i:48 + 16 * i],
                                     lhsT=ident_sb, rhs=ab_bm,
                                     start=True, stop=True,
                                     skip_group_check=True)
                abT_sb = sc("abT_sb", (16, P), bf16)
                V.tensor_copy(abT_sb, abT_ps)
                nc.sync.dma_start(out=ab_d[0, :].rearrange("(j b) -> j b", j=8),
                                  in_=abT_sb[0:8, :])
                nc.sync.dma_start(out=ab_d[1, :].rearrange("(j b) -> j b", j=8),
                                  in_=abT_sb[8:16, :])
                for hsl in (slice(0, 512), slice(512, BL)):
                    nc.scalar.dma_start(out=alpha_b[:, hsl],
                                        in_=ab_d[0:1, hsl].to_broadcast([P, 512]))
                    nc.sync.dma_start(out=beta_b[:, hsl],
                                      in_=ab_d[1:2, hsl].to_broadcast([P, 512]))
                for i in range(2):
                    nc.tensor.matmul(dum_ps, lhsT=ident_sb,
                                     rhs=(alpha_b[:, 0:384] if i == 0
                                          else beta_b[:, 512:896]),
                                     start=(i == 0), stop=(i == 1),
                                     skip_group_check=True)
                # anchor all warm-keeper matmuls against DCE
                dum_sb = sc("dum_sb", (1, 128), bf16)
                V.tensor_copy(dum_sb, stat_ps[0:1, 24:152])
                dum_d = dram.tile([1, 128], bf16, name="dum_d")
                nc.gpsimd.dma_start(out=dum_d, in_=dum_sb)

        # psum pools (mm, stp) released here

        # ---------- z = alpha*h + beta*u (overwrites uT in place),
        # then out = z @ Wo.  Processed in two batch-column halves so the
        # MMo matmuls of half 0 overlap the DVE z-combine of half 1.
        with ExitStack() as ph2:
            mmo = ph2.enter_context(tc.tile_pool(name="mmo", bufs=8,
                                                 space="PSUM"))
            # z-combines for BOTH halves first, so the DVE never head-of-line
            # blocks mmo1's z behind bg0's psum drains
            for bg in range(2):
                csl = slice(bg * 512, (bg + 1) * 512)
                with nc.named_scope(f"zcomb{bg}"):
                    for kh in range(KH):
                        t1z = zzp.tile([P, 512], bf16, name="t1z", tag="zz")
                        V.tensor_mul(t1z, hT_sb[:, kh, csl], alpha_b[:, csl])
                        t2z = zzp.tile([P, 512], bf16, name="t2z", tag="zz")
                        V.tensor_mul(t2z, uT_sb[:, kh, csl], beta_b[:, csl])
                        V.tensor_add(uT_sb[:, kh, csl], t1z, t2z)
            for bg in range(2):
                with nc.named_scope(f"mmo{bg}"):
                    pso = [mmo.tile([P, 500], f32, name=f"pso{bg}_{i}",
                                    tag="mmo") for i in range(8)]
                    for kh in range(KH):
                        wot = wop.tile([P, OUT], bf16, name="wot", tag="wo")
                        nc.sync.dma_start(out=wot, in_=wo_d[kh])
                        for i in range(4):
                            b = bg * 4 + i
                            for och in range(2):
                                nc.tensor.matmul(
                                    pso[i * 2 + och],
                                    lhsT=uT_sb[:, kh, b * P:(b + 1) * P],
                                    rhs=wot[:, och * 500:(och + 1) * 500],
                                    start=(kh == 0), stop=(kh == KH - 1))
                    for i in range(4):
                        b = bg * 4 + i
                        for och in range(2):
                            # stage drains across the scr ring and the (now
                            # dead) alpha/beta tiles for 4-deep DMA pipelining
                            slot = (i * 2 + och) % 3
                            if slot == 0:
                                ob = scr.tile([P, 500], f32, name="ob",
                                              tag="hh")
                            elif slot == 1:
                                ob = abp.tile([P, 500], f32, name="ob",
                                              tag="alpha_b")
                            else:
                                ob = abp.tile([P, 500], f32, name="ob",
                                              tag="beta_b")
                            # bg0 drains stay off the DVE (it is mid-zcomb1
                            # and would head-of-line block); ACT frees banks
                            if bg == 0 or och == 0:
                                S.copy(ob, pso[i * 2 + och])
                            else:
                                V.tensor_copy(ob, pso[i * 2 + 1])
                            outq = nc.sync if och == 0 else nc.scalar
                            outq.dma_start(
                                out=out_d[b * P:(b + 1) * P,
                                          och * 500:(och + 1) * 500],
                                in_=ob)

    nc.compile()
    return nc


def _get_nc(with_b1, with_b2):
    for k, v in _nc_cache:
        if k == (with_b1, with_b2):
            return v
    nc = _build(with_b1, with_b2)
    _nc_cache.append(((with_b1, with_b2), nc))
    return nc


def kernel(x, W1, b1, W2, b2, Wo, bo, cp_w1, cp_b1, cp_w2, cp_b2,
           _trace=False, _tmpdir=None):
    x = np.asarray(x, dtype=np.float32)
    with_b1 = bool(np.any(b1))
    with_b2 = bool(np.any(b2))
    nc = _get_nc(with_b1, with_b2)

    # W1 pre-scaled x1024 so fp8 and bf16 parts share one accumulator
    w1s = np.asarray(W1, np.float32) * W2S
    # fp8 rows 0:1024 in DoubleRow pair layout [KH, P, KQ//2, 2, P]
    w1q_t = np.ascontiguousarray(
        np.clip(w1s[:KQ * P], -E4MAX, E4MAX)
        .reshape(KQ, P, KH, P).transpose(2, 1, 0, 3)
    ).astype(E4).reshape(KH, P, KQ // 2, 2, P)
    # bf16 rows [KH, P, KB, P]
    w1_t = np.ascontiguousarray(
        w1s[KQ * P:].reshape(KB, P, KH, P).transpose(2, 1, 0, 3)
    ).astype(BF)
    # w2r[mh, p, kh, q] = W2[kh*128+p, mh*128+q], fp8 e4m3 scaled x1024
    w2_t = np.ascontiguousarray(
        np.clip(np.asarray(W2, np.float32) * W2S, -E4MAX, E4MAX)
        .reshape(KH, P, KH, P).transpose(2, 1, 0, 3)
    ).astype(E4)
    wo_t = np.asarray(Wo, np.float32).reshape(KH, P, OUT).astype(BF)
    cpw1_t = np.ascontiguousarray(np.clip(
        np.asarray(cp_w1, np.float32).T * 64.0, -E4MAX, E4MAX)
        .reshape(KI, P, 16)).astype(E4)
    cpw2_t = np.asarray(cp_w2, np.float32).reshape(1, 16).T.astype(BF)
    cpw2_t = np.ascontiguousarray(cpw2_t)
    cpb1_t = np.asarray(cp_b1, np.float32).reshape(16, 1)
    cpb2_t = np.asarray(cp_b2, np.float32).reshape(1, 1)
    b1_t = np.ascontiguousarray(np.asarray(b1, np.float32).reshape(KH, P).T)
    b2_t = np.ascontiguousarray(np.asarray(b2, np.float32).reshape(KH, P).T)

    in_maps = []
    for c in range(N_CORES):
        shard = x[c * BL:(c + 1) * BL]
        shT = np.ascontiguousarray(shard.T)
        xq_c = np.clip(shT[:KQ * P], -E4MAX, E4MAX).reshape(KQ, P, BL).astype(E4)
        xT = shT[KQ * P:].reshape(KB, P, BL).astype(BF)
        m = {"xq": xq_c, "xT": xT, "w1": w1_t, "w1q": w1q_t, "w2": w2_t,
             "wo": wo_t,
             "cpw1": cpw1_t, "cpw2": cpw2_t, "cpb1": cpb1_t, "cpb2": cpb2_t,
             "ident": _IDENT}
        if with_b1:
            m["b1"] = b1_t
        if with_b2:
            m["b2"] = b2_t
        in_maps.append(m)

    kw = {}
    if _trace:
        kw = dict(trace=True, tmpdir=_tmpdir or tempfile.mkdtemp(prefix="cdk_"))
    res = run_bass_kernel_spmd(nc, in_maps, list(range(N_CORES)), **kw)

    out = np.concatenate([res.results[c]["out"] for c in range(N_CORES)], axis=0)
    bo = np.asarray(bo, np.float32)
    if np.any(bo):
        out = out + bo
    if _trace:
        kernel._last_result = res
    return out


# revision 40
# speedup vs baseline: 1.5908x; 1.0486x over previous
"""Trainium2 Bass kernel for nn_ConservativeDynamicCurvatureMLP.

Data-parallel over 8 NeuronCores: the batch (8192) is sharded into 8
local shards of 1024 rows; all weights are replicated. The curvature
scalar c_avg couples the shards through a global mean, handled with a
single-scalar AllReduce.

Math (reference):
    h = tanh(x @ W1 + b1)
    u = sigmoid(h @ W2 + b2)
    c = clip(mean(MIN_C + (MAX_C-MIN_C) * sigmoid(relu(x@cp_w1.T+cp_b1)@cp_w2.T+cp_b2)), MIN_C, MAX_C)
    z = poincare_ball_layer(h, u, c, T)
    out = z @ Wo + bo

The poincare layer collapses algebraically to z = alpha(row)*h + beta(row)*u
where alpha/beta are scalar functions of the row statistics
x2=||h||^2, y2=||u||^2, xy=<h,u> and c.  The NaN fallback is measure-zero
and omitted.

Perf notes (measured on hw; baseline 1177us -> ~760us):
  - With 8 cores active the PE is power-throttled to ~2.0 GHz (HAM state
    31), so N=512 matmuls pace at ~263 ns regardless of dtype.  The only
    lever is fewer PE cycles: fp8-e4m3 DoubleRow matmuls (256-deep
    contraction per pass, 2x FLOP rate) carry all of MM2 and the first
    1792 of MM1's 3072 contraction features.  Weights are host-converted
    to e4m3 * 1024 (clipped to +-240, the TRN e4m3 max; W1's bf16 rows
    are scaled x1024 too so both parts share one psum accumulator, the
    1/1024 folds into the activation scale).  h is cast bf16->fp8 per
    row-tile on the DVE for MM2's rhs.  Pushing more features to fp8
    fails the 2e-2 gate (full-fp8 MM1+MM2 simulates at 1.97e-2; this
    split measures 1.723e-2, and err^2 grows linearly with the fp8
    ki-count at ~0.095e-4 per group).  MMo stays bf16 (its quantization
    error hits the output directly).  The fp8 x rides in the padded
    tail of the xT/uT SBUF slot via an AP bitcast.
  - Row statistics go batch-major directly via tiny stationary-squares
    matmuls (lhsT = accumulated squares block, rhs = ones) instead of a
    feature-major ones-reduction + DRAM bounce; the per-row scalar chain
    runs once on [128, 8] tiles, and alpha/beta return to feature-major
    broadcast form via a PE transpose + an 8-descriptor DMA bounce.
  - HAM re-throttles the PE to 1.2 GHz after ~3.4us idle, so warm-keeper
    matmuls (a real accumulation group - DCE-proof - plus tiny taps
    pinned to chain tensors) bridge the stats/chain/broadcast window;
    the z-combines for both batch halves are emitted before the MMo
    blocks so bg0's psum drains never head-of-line-block zcomb1 on the
    DVE, and drains stage through dead alpha/beta tiles for 4-deep DMA
    pipelining.
"""

import tempfile
from contextlib import ExitStack

import numpy as np
import ml_dtypes

import concourse.bass as bass
import concourse.bacc as bacc
import concourse.mybir as mybir
import concourse.tile as tile
from concourse.bass_utils import run_bass_kernel_spmd

P = 128
N_CORES = 8
B_FULL = 8192
BL = B_FULL // N_CORES          # 1024 rows per core
IN = 3072
HID = 4096
OUT = 1000
KI = IN // P                    # 24
KQ = 16                         # leading ki-groups of MM1 done in fp8
KB = KI - KQ                    # trailing ki-groups in bf16
KH = HID // P                   # 32
NB = BL // P                    # 8 batch tiles
MIN_C = 0.001 * 0.5
MAX_C = 0.001 * 2.0
T_CONST = 0.7
EPS = 1e-7
W2S = 1024.0                    # host-side fp8 scale for W2
E4MAX = 240.0                   # TRN e4m3 saturation

dt = mybir.dt
AF = mybir.ActivationFunctionType
ALU = mybir.AluOpType
PM = mybir.MatmulPerfMode
BF = ml_dtypes.bfloat16
E4 = ml_dtypes.float8_e4m3

_nc_cache = []
_IDENT = np.eye(P, dtype=ml_dtypes.bfloat16)


def _build(with_b1, with_b2):
    nc = bacc.Bacc("TRN2", target_bir_lowering=False, debug=False,
                   num_devices=N_CORES)

    # x features 0:1024 as fp8 (partial-fp8 MM1), 1024:3072 as bf16
    xq_d = nc.dram_tensor("xq", [KQ, P, BL], dt.float8e4, kind="ExternalInput")
    xT_d = nc.dram_tensor("xT", [KB, P, BL], dt.bfloat16, kind="ExternalInput")
    # W1 scaled x1024 throughout (so fp8 and bf16 parts share one psum
    # accumulator); fp8 rows in DoubleRow pair layout, bf16 rows in halves
    w1q_d = nc.dram_tensor("w1q", [KH, P, KQ // 2, 2, P], dt.float8e4,
                           kind="ExternalInput")
    w1_d = nc.dram_tensor("w1", [KH, P, KB, P], dt.bfloat16,
                          kind="ExternalInput")
    # W2 in fp8 (x1024): w2r[mh, p, kh, q] = W2[kh*128+p, mh*128+q]
    w2_d = nc.dram_tensor("w2", [KH, P, KH, P], dt.float8e4, kind="ExternalInput")
    wo_d = nc.dram_tensor("wo", [KH, P, OUT], dt.bfloat16, kind="ExternalInput")
    cpw1_d = nc.dram_tensor("cpw1", [KI, P, 16], dt.float8e4, kind="ExternalInput")
    cpw2_d = nc.dram_tensor("cpw2", [16, 1], dt.bfloat16, kind="ExternalInput")
    cpb1_d = nc.dram_tensor("cpb1", [16, 1], dt.float32, kind="ExternalInput")
    cpb2_d = nc.dram_tensor("cpb2", [1, 1], dt.float32, kind="ExternalInput")
    ident_d = nc.dram_tensor("ident", [P, P], dt.bfloat16, kind="ExternalInput")
    b1_d = nc.dram_tensor("b1", [P, KH], dt.float32, kind="ExternalInput") if with_b1 else None
    b2_d = nc.dram_tensor("b2", [P, KH], dt.float32, kind="ExternalInput") if with_b2 else None
    out_d = nc.dram_tensor("out", [BL, OUT], dt.float32, kind="ExternalOutput")

    f32 = dt.float32
    bf16 = dt.bfloat16
    fp8 = dt.float8e4

    with tile.TileContext(nc) as tc, ExitStack() as ctx:
        const = ctx.enter_context(tc.tile_pool(name="const", bufs=1))
        big = ctx.enter_context(tc.tile_pool(name="big", bufs=1))
        htp = ctx.enter_context(tc.tile_pool(name="htp", bufs=1))
        hqp = ctx.enter_context(tc.tile_pool(name="hqp", bufs=1))
        wp = ctx.enter_context(tc.tile_pool(name="wp", bufs=2))
        wop = ctx.enter_context(tc.tile_pool(name="wop", bufs=4))
        scr = ctx.enter_context(tc.tile_pool(name="scr", bufs=2))
        zzp = ctx.enter_context(tc.tile_pool(name="zzp", bufs=3))
        sacc = ctx.enter_context(tc.tile_pool(name="sacc", bufs=1))
        abp = ctx.enter_context(tc.tile_pool(name="abp", bufs=1))
        scal = ctx.enter_context(tc.tile_pool(name="scal", bufs=1))
        cpp = ctx.enter_context(tc.tile_pool(name="cpp", bufs=1))
        dram = ctx.enter_context(tc.tile_pool(name="dram", bufs=1, space="DRAM"))

        V = nc.vector
        S = nc.scalar

        def sc(name, shape=(P, 8), dtype=f32):
            return scal.tile(list(shape), dtype, name=name, tag=name)

        # ---------- persistent activations (feature-major) ----------
        onesb = const.tile([P, 1], bf16, name="onesb")
        nc.vector.memset(onesb, 1.0)
        onesf = const.tile([P, 1], f32, name="onesf")
        nc.vector.memset(onesf, 1.0)
        # one 64KB slot holds bf16 x (ki 0:KB), fp8 x (as bitcast bytes in
        # the tail), and is later recycled as uT
        xall = big.tile([P, KH, BL], bf16, name="xall", tag="big")
        xT_sb = xall[:, 0:KB, :]
        # first weight half-row issues on the sync queue before anything else;
        # xT streams on the gpsimd + scalar queues in parallel (the load is
        # HBM-bandwidth-bound, ~35us for 6.3MB with all 8 cores pulling)
        w1q0 = wp.tile([P, KQ // 2, 2, P], fp8, name="w1qrow", tag="w1q")
        nc.sync.dma_start(out=w1q0, in_=w1q_d[0])
        # mh=0's bf16 row must beat the bulk x chunks into the sync queue,
        # or mm1 stalls right after its fp8 head
        w1r0 = wp.tile([P, KB, P], bf16, name="w1row", tag="w1")
        nc.sync.dma_start(out=w1r0, in_=w1_d[0])
        w1q1 = wp.tile([P, KQ // 2, 2, P], fp8, name="w1qrow", tag="w1q")
        nc.scalar.dma_start(out=w1q1, in_=w1q_d[1])
        w1r1 = wp.tile([P, KB, P], bf16, name="w1row", tag="w1")
        nc.sync.dma_start(out=w1r1, in_=w1_d[1])
        xq_sb = (xall[:, KB:KB + KQ // 2, :].bitcast(fp8)
                 .rearrange("p a (t b) -> p (a t) b", t=2))
        queues = [nc.scalar, nc.sync]
        for idx, (a, b) in enumerate(((0, 2), (2, 4), (4, 7), (7, 10),
                                      (10, 13), (13, 16))):
            queues[idx % 2].dma_start(
                out=xq_sb[:, a:b, :],
                in_=xq_d[a:b].rearrange("k p b -> p k b"))
        for idx, (a, b) in enumerate(((0, 2), (2, 4), (4, 6), (6, 8))):
            queues[idx % 2].dma_start(
                out=xT_sb[:, a:b, :],
                in_=xT_d[a:b].rearrange("k p b -> p k b"))
        ident_sb = const.tile([P, P], bf16, name="ident_sb")
        nc.sync.dma_start(out=ident_sb, in_=ident_d[:, :])
        hT_sb = htp.tile([P, KH, BL], bf16, name="hT_sb")
        hTq_sb = hqp.tile([P, KH, BL], fp8, name="hTq_sb")
        if with_b1:
            b1_sb = const.tile([P, KH], f32, name="b1_sb")
            nc.sync.dma_start(out=b1_sb, in_=b1_d[:, :])
        if with_b2:
            b2_sb = const.tile([P, KH], f32, name="b2_sb")
            nc.sync.dma_start(out=b2_sb, in_=b2_d[:, :])

        with ExitStack() as ph1:
            mm = ph1.enter_context(tc.tile_pool(name="mm", bufs=3, space="PSUM"))
            stp = ph1.enter_context(tc.tile_pool(name="stp", bufs=1, space="PSUM"))
            # one bank shared by the batch-major stats (cols 0:8 x2,
            # 8:16 y2, 16:24 xy) and the HAM warm-keeper matmul target
            # (cols 128:512); plus the a/b transpose target bank
            stat_ps = stp.tile([P, 512], f32, name="stat_ps")
            dum_ps = stat_ps[:, 128:512]
            abT_ps = stp.tile([16, P], bf16, name="abT_ps")

            # ---------- MM1: hT = tanh(W1.T @ xT) (bf16) ----------
            x2a = sacc.tile([P, BL], bf16, name="x2a")
            y2a = sacc.tile([P, BL], bf16, name="y2a")
            xya = sacc.tile([P, BL], bf16, name="xya")
            with nc.named_scope("mm1"):
                for mh in range(KH):
                    ps = mm.tile([P, BL], f32, name="ps", tag="mm")
                    if mh == 0:
                        w1q, w1r = w1q0, w1r0
                    elif mh == 1:
                        w1q, w1r = w1q1, w1r1
                    else:
                        w1q = wp.tile([P, KQ // 2, 2, P], fp8, name="w1qrow",
                                      tag="w1q")
                        nc.scalar.dma_start(out=w1q, in_=w1q_d[mh])
                        w1r = wp.tile([P, KB, P], bf16, name="w1row", tag="w1")
                        nc.sync.dma_start(out=w1r, in_=w1_d[mh])
                    for hsl in (slice(0, 512), slice(512, BL)):
                        for j in range(KQ // 2):
                            nc.tensor.matmul(
                                ps[:, hsl], lhsT=w1q[:, j, :, :],
                                rhs=xq_sb[:, 2 * j:2 * j + 2, hsl],
                                start=(j == 0), stop=False,
                                perf_mode=PM.DoubleRow,
                                skip_group_check=True)
                        for ki in range(KB):
                            nc.tensor.matmul(ps[:, hsl], lhsT=w1r[:, ki, :],
                                             rhs=xT_sb[:, ki, hsl],
                                             start=False, stop=(ki == KB - 1),
                                             skip_group_check=True)
                    if with_b1:
                        S.activation(hT_sb[:, mh, :], ps, AF.Tanh,
                                     bias=b1_sb[:, mh:mh + 1], scale=1.0 / W2S)
                    else:
                        S.activation(hT_sb[:, mh, :], ps, AF.Tanh,
                                     scale=1.0 / W2S)
                    # fp8 copy for MM2's DoubleRow rhs
                    V.tensor_copy(hTq_sb[:, mh, :], hT_sb[:, mh, :])
                    hh = scr.tile([P, BL], bf16, name="hh", tag="hh")
                    S.activation(hh, hT_sb[:, mh, :], AF.Square)
                    if mh == 0:
                        V.tensor_copy(x2a, hh)
                    else:
                        V.tensor_add(x2a, x2a, hh)
                # x2 -> batch-major: out[b,0] = sum_p x2a[p, blk*128+b]
                for blk in range(NB):
                    nc.tensor.matmul(stat_ps[:, blk:blk + 1],
                                     lhsT=x2a[:, blk * P:(blk + 1) * P],
                                     rhs=onesb, start=True, stop=True,
                                     skip_group_check=True)

            # ---------- curvature predictor (xT still resident; the
            # AllReduce hides under MM1/MM2) ----------
            with nc.named_scope("cp"):
                cpw1_sb = const.tile([P, KI, 16], fp8, name="cpw1_sb")
                nc.sync.dma_start(out=cpw1_sb,
                                  in_=cpw1_d.rearrange("k p q -> p k q"))
                cpw2_sb = const.tile([16, 1], bf16, name="cpw2_sb")
                nc.sync.dma_start(out=cpw2_sb, in_=cpw2_d[:, :])
                cpb1_sb = const.tile([16, 1], f32, name="cpb1_sb")
                nc.sync.dma_start(out=cpb1_sb, in_=cpb1_d[:, :])
                cpb2_sb = const.tile([1, 1], f32, name="cpb2_sb")
                nc.sync.dma_start(out=cpb2_sb, in_=cpb2_d[:, :])
                cph_sb = cpp.tile([16, BL], fp8, name="cph_sb")
                for ch in range(2):
                    cps = mm.tile([16, 512], f32, name="cps", tag="mm")
                    for ki in range(KI):
                        xr = (xq_sb[:, ki, ch * 512:(ch + 1) * 512] if ki < KQ
                              else xT_sb[:, ki - KQ, ch * 512:(ch + 1) * 512])
                        nc.tensor.matmul(
                            cps, lhsT=cpw1_sb[:, ki, :], rhs=xr,
                            start=(ki == 0), stop=(ki == KI - 1))
                    S.activation(cph_sb[:, ch * 512:(ch + 1) * 512], cps,
                                 AF.Relu, bias=cpb1_sb, scale=1.0 / 64.0)
                sparts = []
                for ch in range(2):
                    c2p = mm.tile([1, 512], f32, name="c2p", tag="mm")
                    nc.tensor.matmul(c2p, lhsT=cpw2_sb,
                                     rhs=cph_sb[:16, ch * 512:(ch + 1) * 512],
                                     start=True, stop=True)
                    cpw = cpp.tile([1, 512], bf16, name="cpw", tag="cpw")
                    spart = cpp.tile([1, 1], f32, name=f"spart{ch}",
                                     tag=f"spart{ch}")
                    S.activation(cpw, c2p, AF.Sigmoid, bias=cpb2_sb,
                                 accum_out=spart)
                    sparts.append(spart)
                s_loc = cpp.tile([1, 1], f32, name="s_loc")
                V.tensor_add(s_loc, sparts[0], sparts[1])
                cin = dram.tile([1, 1], f32, name="cin")
                cout = dram.tile([1, 1], f32, name="cout")
                nc.sync.dma_start(out=cin, in_=s_loc)
                nc.gpsimd.collective_compute(
                    "AllReduce", ALU.add,
                    replica_groups=[list(range(N_CORES))],
                    ins=[cin.opt()], outs=[cout.opt()])
                s_b = sc("s_b", (P, 1))
                nc.gpsimd.dma_start(out=s_b, in_=cout.to_broadcast([P, 1]))
                # c = clip(MIN_C + (MAX_C-MIN_C)*mean(c_pred))
                c_b = sc("c_b", (P, 1))
                V.tensor_scalar(out=c_b, in0=s_b,
                                scalar1=(MAX_C - MIN_C) / B_FULL,
                                scalar2=MIN_C, op0=ALU.mult, op1=ALU.add)
                V.tensor_scalar_min(out=c_b, in0=c_b, scalar1=MAX_C)
                V.tensor_scalar_max(out=c_b, in0=c_b, scalar1=MIN_C)
                negc_b = sc("negc_b", (P, 1))
                V.tensor_scalar_mul(out=negc_b, in0=c_b, scalar1=-1.0)
                twoc_b = sc("twoc_b", (P, 1))
                V.tensor_scalar_mul(out=twoc_b, in0=c_b, scalar1=2.0)
                neg2c_b = sc("neg2c_b", (P, 1))
                V.tensor_scalar_mul(out=neg2c_b, in0=c_b, scalar1=-2.0)
                c2_b = sc("c2_b", (P, 1))
                V.tensor_mul(c2_b, c_b, c_b)

            # ---------- MM2: uT = sigmoid(W2.T @ hT) fp8 DoubleRow ----------
            uT_sb = big.tile([P, KH, BL], bf16, name="uT_sb", tag="big")
            inv_s2 = 1.0 / W2S
            with nc.named_scope("mm2"):
                for mh in range(KH):
                    ps = mm.tile([P, BL], f32, name="ps", tag="mm")
                    w2row = wp.tile([P, KH, P], fp8, name="w2row", tag="w2")
                    nc.sync.dma_start(out=w2row, in_=w2_d[mh])
                    for j in range(KH // 2):
                        nc.tensor.matmul(ps[:, 0:512],
                                         lhsT=w2row[:, 2 * j:2 * j + 2, :],
                                         rhs=hTq_sb[:, 2 * j:2 * j + 2, 0:512],
                                         start=(j == 0), stop=(j == KH // 2 - 1),
                                         perf_mode=PM.DoubleRow)
                        nc.tensor.matmul(ps[:, 512:BL],
                                         lhsT=w2row[:, 2 * j:2 * j + 2, :],
                                         rhs=hTq_sb[:, 2 * j:2 * j + 2, 512:BL],
                                         start=(j == 0), stop=(j == KH // 2 - 1),
                                         perf_mode=PM.DoubleRow)
                    if with_b2:
                        S.activation(uT_sb[:, mh, :], ps, AF.Sigmoid,
                                     bias=b2_sb[:, mh:mh + 1], scale=inv_s2)
                    else:
                        S.activation(uT_sb[:, mh, :], ps, AF.Sigmoid,
                                     scale=inv_s2)
                    uu = scr.tile([P, BL], bf16, name="uu", tag="hh")
                    S.activation(uu, uT_sb[:, mh, :], AF.Square)
                    hu = scr.tile([P, BL], bf16, name="hu", tag="hh")
                    V.tensor_mul(hu, hT_sb[:, mh, :], uT_sb[:, mh, :])
                    if mh == 0:
                        V.tensor_copy(y2a, uu)
                        V.tensor_copy(xya, hu)
                    else:
                        V.tensor_add(y2a, y2a, uu)
                        V.tensor_add(xya, xya, hu)
                # warm-keeper group 1: bridges the PE from mm2's last matmul
                # across the stats-accumulation drain (a >3.4us idle would
                # HAM-throttle the clock for the MMo matmuls).  A real
                # accumulation group so DCE cannot drop it.
                for i in range(18):
                    nc.tensor.matmul(dum_ps, lhsT=ident_sb,
                                     rhs=uT_sb[:, KH - 1, 0:384],
                                     start=(i == 0), stop=(i == 17),
                                     skip_group_check=True)
                for blk in range(NB):
                    nc.tensor.matmul(stat_ps[:, 8 + blk:9 + blk],
                                     lhsT=y2a[:, blk * P:(blk + 1) * P],
                                     rhs=onesb, start=True, stop=True,
                                     skip_group_check=True)
                    nc.tensor.matmul(stat_ps[:, 16 + blk:17 + blk],
                                     lhsT=xya[:, blk * P:(blk + 1) * P],
                                     rhs=onesb, start=True, stop=True,
                                     skip_group_check=True)

            # ---------- per-row scalar chain, batch-major [128, 8] ----------
            alpha_b = abp.tile([P, BL], bf16, name="alpha_b", tag="alpha_b")
            beta_b = abp.tile([P, BL], bf16, name="beta_b", tag="beta_b")
            ab_d = dram.tile([2, BL], bf16, name="ab_d")

            with nc.named_scope("scalars"):
                stats_sb = sc("stats_sb", (P, 24))
                V.tensor_copy(stats_sb, stat_ps[:, 0:24])
                x2 = stats_sb[:, 0:8]
                y2 = stats_sb[:, 8:16]
                xy = stats_sb[:, 16:24]
                w = sc("w")
                V.scalar_tensor_tensor(out=w, in0=xy, scalar=-2.0, in1=y2,
                                       op0=ALU.mult, op1=ALU.add)
                A1 = sc("A1")
                V.tensor_scalar(out=A1, in0=w, scalar1=c_b, scalar2=1.0,
                                op0=ALU.mult, op1=ALU.add)
                A2 = sc("A2")
                V.tensor_scalar(out=A2, in0=x2, scalar1=negc_b, scalar2=1.0,
                                op0=ALU.mult, op1=ALU.add)
                p1 = sc("p1")
                V.tensor_mul(p1, x2, y2)
                den = sc("den")
                V.tensor_scalar(out=den, in0=p1, scalar1=c2_b, scalar2=1.0,
                                op0=ALU.mult, op1=ALU.add)
                V.scalar_tensor_tensor(out=den, in0=xy, scalar=neg2c_b, in1=den,
                                       op0=ALU.mult, op1=ALU.add)
                V.tensor_scalar_add(out=den, in0=den, scalar1=EPS)
                D = sc("D")
                V.reciprocal(D, den)
                nc.tensor.matmul(stat_ps[0:1, 80:88], lhsT=onesf, rhs=D,
                                 start=True, stop=True, skip_group_check=True)
                # ||a||^2 = D^2 (A1^2 x2 - 2 A1 A2 xy + A2^2 y2)
                t1 = sc("t1")
                V.tensor_mul(t1, A1, A1)
                V.tensor_mul(t1, t1, x2)
                t2 = sc("t2")
                V.tensor_mul(t2, A1, A2)
                V.tensor_mul(t2, t2, xy)
                t3 = sc("t3")
                V.tensor_mul(t3, A2, A2)
                V.tensor_mul(t3, t3, y2)
                na2 = sc("na2")
                V.scalar_tensor_tensor(out=na2, in0=t2, scalar=-2.0, in1=t1,
                                       op0=ALU.mult, op1=ALU.add)
                V.tensor_add(na2, na2, t3)
                dsq = sc("dsq")
                V.tensor_mul(dsq, D, D)
                V.tensor_mul(na2, na2, dsq)
                # q = sqrt(c * na2) with one Newton step (ACT sqrt is low precision)
                q2 = sc("q2")
                V.tensor_scalar(out=q2, in0=na2, scalar1=c_b, scalar2=None,
                                op0=ALU.mult)
                q0 = sc("q0")
                S.activation(q0, q2, AF.Sqrt)
                V.tensor_scalar_max(out=q0, in0=q0, scalar1=1e-20)
                nc.tensor.matmul(stat_ps[0:1, 96:104], lhsT=onesf, rhs=q0,
                                 start=True, stop=True, skip_group_check=True)
                r0 = sc("r0")
                V.reciprocal(r0, q0)
                q = sc("q")
                V.tensor_mul(q, q2, r0)
                V.tensor_add(q, q, q0)
                V.tensor_scalar_mul(out=q, in0=q, scalar1=0.5)
                arg = sc("arg")
                V.tensor_scalar_min(out=arg, in0=q, scalar1=1.0 - 1e-5)
                # artanh(arg) = 0.5 ln((1+arg)/(1-arg)); t = tanh(T*artanh)/q
                opp = sc("opp")
                V.tensor_scalar(out=opp, in0=arg, scalar1=-1.0, scalar2=1.0,
                                op0=ALU.mult, op1=ALU.add)
                opn = sc("opn")
                V.tensor_scalar_add(out=opn, in0=arg, scalar1=1.0)
                rr = sc("rr")
                V.reciprocal(rr, opp)
                rat = sc("rat")
                V.tensor_mul(rat, opn, rr)
                lg = sc("lg")
                S.activation(lg, rat, AF.Ln)
                th = sc("th")
                S.activation(th, lg, AF.Tanh, scale=T_CONST * 0.5)
                rq = sc("rq")
                V.reciprocal(rq, q)
                tm = sc("tm")
                V.tensor_mul(tm, th, rq)
                nc.tensor.matmul(stat_ps[0:1, 88:96], lhsT=onesf, rhs=tm,
                                 start=True, stop=True, skip_group_check=True)
                # <h,a> = D (A2 xy - A1 x2)
                s1_ = sc("s1_")
                V.tensor_mul(s1_, A1, x2)
                s2_ = sc("s2_")
                V.tensor_mul(s2_, A2, xy)
                ha = sc("ha")
                V.tensor_sub(ha, s2_, s1_)
                V.tensor_mul(ha, ha, D)
                nc.tensor.matmul(stat_ps[0:1, 104:112], lhsT=onesf, rhs=ha,
                                 start=True, stop=True, skip_group_check=True)
                hm = sc("hm")
                V.tensor_mul(hm, tm, ha)
                tsq = sc("tsq")
                V.tensor_mul(tsq, tm, tm)
                m2 = sc("m2")
                V.tensor_mul(m2, tsq, na2)
                w2s = sc("w2s")
                V.scalar_tensor_tensor(out=w2s, in0=hm, scalar=2.0, in1=m2,
                                       op0=ALU.mult, op1=ALU.add)
                B1 = sc("B1")
                V.tensor_scalar(out=B1, in0=w2s, scalar1=c_b, scalar2=1.0,
                                op0=ALU.mult, op1=ALU.add)
                p2 = sc("p2")
                V.tensor_mul(p2, x2, m2)
                den2 = sc("den2")
                V.tensor_scalar(out=den2, in0=p2, scalar1=c2_b, scalar2=1.0,
                                op0=ALU.mult, op1=ALU.add)
                V.scalar_tensor_tensor(out=den2, in0=hm, scalar=twoc_b, in1=den2,
                                       op0=ALU.mult, op1=ALU.add)
                V.tensor_scalar_add(out=den2, in0=den2, scalar1=EPS)
                D2 = sc("D2")
                V.reciprocal(D2, den2)
                g = sc("g")
                V.tensor_mul(g, A2, tm)
                V.tensor_mul(g, g, D)
                w3 = sc("w3")
                V.tensor_mul(w3, g, A1)
                V.tensor_sub(w3, B1, w3)
                # alpha -> cols 0:8, beta -> cols 8:16 of one bf16 tile; a PE
                # transpose then yields batch-linear rows for a fast DMA
                # bounce (8 contiguous 256B descriptors instead of a 2-byte
                # element scatter).
                ab_bm = sc("ab_bm", (P, 16), bf16)
                V.tensor_mul(ab_bm[:, 0:8], w3, D2)
                w4 = sc("w4")
                V.tensor_mul(w4, g, A2)
                V.tensor_mul(ab_bm[:, 8:16], w4, D2)
                nc.tensor.transpose(abT_ps, ab_bm, ident_sb)
                # warm-keeper taps pinned after the chain end: the scheduler
                # hoists dependency-free matmuls, so these read ab_bm
                for i in range(3):
                    nc.tensor.matmul(stat_ps[:, 32 + 16 * i:48 + 16 * i],
                                     lhsT=ident_sb, rhs=ab_bm,
                                     start=True, stop=True,
                                     skip_group_check=True)
                abT_sb = sc("abT_sb", (16, P), bf16)
                V.tensor_copy(abT_sb, abT_ps)
                nc.sync.dma_start(out=ab_d[0, :].rearrange("(j b) -> j b", j=8),
                                  in_=abT_sb[0:8, :])
                nc.sync.dma_start(out=ab_d[1, :].rearrange("(j b) -> j b", j=8),
                                  in_=abT_sb[8:16, :])
                for hsl in (slice(0, 512), slice(512, BL)):
                    nc.scalar.dma_start(out=alpha_b[:, hsl],
                                        in_=ab_d[0:1, hsl].to_broadcast([P, 512]))
                    nc.sync.dma_start(out=beta_b[:, hsl],
                                      in_=ab_d[1:2, hsl].to_broadcast([P, 512]))
                for i in range(2):
                    nc.tensor.matmul(dum_ps, lhsT=ident_sb,
                                     rhs=(alpha_b[:, 0:384] if i == 0
                                          else beta_b[:, 512:896]),
                                     start=(i == 0), stop=(i == 1),
                                     skip_group_check=True)
                # anchor all warm-keeper matmuls against DCE
                dum_sb = sc("dum_sb", (1, 128), bf16)
                V.tensor_copy(dum_sb, stat_ps[0:1, 24:152])
                dum_d = dram.tile([1, 128], bf16, name="dum_d")
                nc.gpsimd.dma_start(out=dum_d, in_=dum_sb)

        # psum pools (mm, stp) released here

        # ---------- z = alpha*h + beta*u (overwrites uT in place),
        # then out = z @ Wo.  Processed in two batch-column halves so the
        # MMo matmuls of half 0 overlap the DVE z-combine of half 1.
        with ExitStack() as ph2:
            mmo = ph2.enter_context(tc.tile_pool(name="mmo", bufs=8,
                                                 space="PSUM"))
            # z-combines for BOTH halves first, so the DVE never head-of-line
            # blocks mmo1's z behind bg0's psum drains
            for bg in range(2):
                csl = slice(bg * 512, (bg + 1) * 512)
                with nc.named_scope(f"zcomb{bg}"):
                    for kh in range(KH):
                        t1z = zzp.tile([P, 512], bf16, name="t1z", tag="zz")
                        V.tensor_mul(t1z, hT_sb[:, kh, csl], alpha_b[:, csl])
                        t2z = zzp.tile([P, 512], bf16, name="t2z", tag="zz")
                        V.tensor_mul(t2z, uT_sb[:, kh, csl], beta_b[:, csl])
                        V.tensor_add(uT_sb[:, kh, csl], t1z, t2z)
            for bg in range(2):
                with nc.named_scope(f"mmo{bg}"):
                    pso = [mmo.tile([P, 500], f32, name=f"pso{bg}_{i}",
                                    tag="mmo") for i in range(8)]
                    for kh in range(KH):
                        wot = wop.tile([P, OUT], bf16, name="wot", tag="wo")
                        nc.sync.dma_start(out=wot, in_=wo_d[kh])
                        for i in range(4):
                            b = bg * 4 + i
                            for och in range(2):
                                nc.tensor.matmul(
                                    pso[i * 2 + och],
                                    lhsT=uT_sb[:, kh, b * P:(b + 1) * P],
                                    rhs=wot[:, och * 500:(och + 1) * 500],
                                    start=(kh == 0), stop=(kh == KH - 1))
                    for i in range(4):
                        b = bg * 4 + i
                        for och in range(2):
                            # stage drains across the scr ring and the (now
                            # dead) alpha/beta tiles for 4-deep DMA pipelining
                            slot = (i * 2 + och) % 3
                            if slot == 0:
                                ob = scr.tile([P, 500], f32, name="ob",
                                              tag="hh")
                            elif slot == 1:
                                ob = abp.tile([P, 500], f32, name="ob",
                                              tag="alpha_b")
                            else:
                                ob = abp.tile([P, 500], f32, name="ob",
                                              tag="beta_b")
                            # bg0 drains stay off the DVE (it is mid-zcomb1
                            # and would head-of-line block); ACT frees banks
                            if bg == 0 or och == 0:
                                S.copy(ob, pso[i * 2 + och])
                            else:
                                V.tensor_copy(ob, pso[i * 2 + 1])
                            outq = nc.sync if och == 0 else nc.scalar
                            outq.dma_start(
                                out=out_d[b * P:(b + 1) * P,
                                          och * 500:(och + 1) * 500],
                                in_=ob)

    nc.compile()
    return nc


def _get_nc(with_b1, with_b2):
    for k, v in _nc_cache:
        if k == (with_b1, with_b2):
            return v
    nc = _build(with_b1, with_b2)
    _nc_cache.append(((with_b1, with_b2), nc))
    return nc


def kernel(x, W1, b1, W2, b2, Wo, bo, cp_w1, cp_b1, cp_w2, cp_b2,
           _trace=False, _tmpdir=None):
    x = np.asarray(x, dtype=np.float32)
    with_b1 = bool(np.any(b1))
    with_b2 = bool(np.any(b2))
    nc = _get_nc(with_b1, with_b2)

    # W1 pre-scaled x1024 so fp8 and bf16 parts share one accumulator
    w1s = np.asarray(W1, np.float32) * W2S
    # fp8 rows 0:1024 in DoubleRow pair layout [KH, P, KQ//2, 2, P]
    w1q_t = np.ascontiguousarray(
        np.clip(w1s[:KQ * P], -E4MAX, E4MAX)
        .reshape(KQ, P, KH, P).transpose(2, 1, 0, 3)
    ).astype(E4).reshape(KH, P, KQ // 2, 2, P)
    # bf16 rows [KH, P, KB, P]
    w1_t = np.ascontiguousarray(
        w1s[KQ * P:].reshape(KB, P, KH, P).transpose(2, 1, 0, 3)
    ).astype(BF)
    # w2r[mh, p, kh, q] = W2[kh*128+p, mh*128+q], fp8 e4m3 scaled x1024
    w2_t = np.ascontiguousarray(
        np.clip(np.asarray(W2, np.float32) * W2S, -E4MAX, E4MAX)
        .reshape(KH, P, KH, P).transpose(2, 1, 0, 3)
    ).astype(E4)
    wo_t = np.asarray(Wo, np.float32).reshape(KH, P, OUT).astype(BF)
    cpw1_t = np.ascontiguousarray(np.clip(
        np.asarray(cp_w1, np.float32).T * 64.0, -E4MAX, E4MAX)
        .reshape(KI, P, 16)).astype(E4)
    cpw2_t = np.asarray(cp_w2, np.float32).reshape(1, 16).T.astype(BF)
    cpw2_t = np.ascontiguousarray(cpw2_t)
    cpb1_t = np.asarray(cp_b1, np.float32).reshape(16, 1)
    cpb2_t = np.asarray(cp_b2, np.float32).reshape(1, 1)
    b1_t = np.ascontiguousarray(np.asarray(b1, np.float32).reshape(KH, P).T)
    b2_t = np.ascontiguousarray(np.asarray(b2, np.float32).reshape(KH, P).T)

    in_maps = []
    for c in range(N_CORES):
        shard = x[c * BL:(c + 1) * BL]
        shT = np.ascontiguousarray(shard.T)
        xq_c = np.clip(shT[:KQ * P], -E4MAX, E4MAX).reshape(KQ, P, BL).astype(E4)
        xT = shT[KQ * P:].reshape(KB, P, BL).astype(BF)
        m = {"xq": xq_c, "xT": xT, "w1": w1_t, "w1q": w1q_t, "w2": w2_t,
             "wo": wo_t,
             "cpw1": cpw1_t, "cpw2": cpw2_t, "cpb1": cpb1_t, "cpb2": cpb2_t,
             "ident": _IDENT}
        if with_b1:
            m["b1"] = b1_t
        if with_b2:
            m["b2"] = b2_t
        in_maps.append(m)

    kw = {}
    if _trace:
        kw = dict(trace=True, tmpdir=_tmpdir or tempfile.mkdtemp(prefix="cdk_"))
    res = run_bass_kernel_spmd(nc, in_maps, list(range(N_CORES)), **kw)

    out = np.concatenate([res.results[c]["out"] for c in range(N_CORES)], axis=0)
    bo = np.asarray(bo, np.float32)
    if np.any(bo):
        out = out + bo
    if _trace:
        kernel._last_result = res
    return out
